# revision 53
# baseline (speedup 1.0000x reference)
"""Self-contained Trainium2 Bass kernel for the CR-VSS block (8 cores)."""

# ---- TileContext drain-wait patch (walrus 1-wait limit) ----
"""Patch TileContext._drain_and_barrier: the axon-client walrus rejects
instructions carrying >2 sem waits ("Too many sync wait commands" in
setupSyncWait for CTRL structs). Redistribute the exit-drain's waits across
preceding SP nop instructions, each carrying at most MAX_WAITS."""
from concourse.tile import TileContext, ScopedClock

MAX_WAITS = 1


def _patched_drain_and_barrier(self, tick_clock, wait_clock):
    nc = self.nc
    drain_inst = nc.sync.drain()
    wait_clock.add_sem_waits(
        drain_inst.ins, ScopedClock({None: tick_clock.global_clock})
    )

    waits = list(drain_inst.ins.sync_info.on_wait or [])
    if len(waits) > MAX_WAITS:
        bb = nc.cur_bb.bb
        assert bb.instructions[-1] is drain_inst.ins
        # strip waits from the drain, re-emit them on nop carriers
        drain_inst.ins.sync_info.on_wait = waits[:0]
        carriers = []
        import concourse.mybir as mybir
        for i in range(0, len(waits), MAX_WAITS):
            nop = nc.sync.nop(nofuse=True)
            nop.ins.sync_info = mybir.SyncInfo(
                on_wait=waits[i:i + MAX_WAITS], on_update=[]
            )
            carriers.append(nop.ins)
        # move carriers before the drain
        insts = list(bb.instructions)
        assert insts[-len(carriers) - 1] is drain_inst.ins
        reordered = insts[:-len(carriers) - 1] + insts[-len(carriers):] + [drain_inst.ins]
        while len(bb.instructions):
            bb.instructions.pop()
        for x in reordered:
            bb.instructions.append(x)

    nc.all_engine_barrier()
    assert self.sems is not None
    popped = nc._tile_sem_poison_stack.pop()
    assert popped is self._sem_poison
    nc.clear_and_free_semaphores(list(self.sems.allocated().values()))
    nc.all_engine_barrier()


def apply():
    TileContext._drain_and_barrier = _patched_drain_and_barrier


def split_multi_waits(nc, max_waits=1):
    """Post-pass: walrus CTRL codegen rejects instructions with more than
    one sem wait. Move extra waits onto same-engine NoOp carriers."""
    import concourse.mybir as mybir
    for f in nc.m.functions:
        for bb in f.blocks:
            insts = list(bb.instructions)
            out = []
            changed = False
            for ins in insts:
                si = ins.sync_info
                if si is not None and si.on_wait and len(si.on_wait) > max_waits:
                    waits = list(si.on_wait)
                    for i, w in enumerate(waits[max_waits:]):
                        nop = mybir.InstNoOp.__new__(
                            mybir.InstNoOp, name=f"{ins.name}-xw{i}", ins=[], outs=[])
                        nop.engine = ins.engine
                        nop.sync_info = mybir.SyncInfo(on_wait=[w], on_update=[])
                        out.append(nop)
                    ins.sync_info = mybir.SyncInfo(
                        on_wait=waits[:max_waits],
                        on_update=list(si.on_update or []))
                    changed = True
                out.append(ins)
            if changed:
                while len(bb.instructions):
                    bb.instructions.pop()
                for x in out:
                    bb.instructions.append(x)

apply()

# ---- kernel ----
"""Trainium2 Bass kernel for nn_CR_VSS (VSS block with SS2D selective scan).

Sharding: 8 cores = 4 samples x 2 d_inner-halves. Each core runs the full
pre-stage for its sample, scans its 96-channel d-half across all 4
cross-scan directions (packed into 3x128-partition tiles), then the pair
exchanges y-halves with ONE AllGather; LN + out-proj + post-stage run
locally (z is computed full-width in the in-proj so no second collective).

Scan: h_t = exp(A*delta_t)*h_{t-1} + delta_t*u_t*B_t per (k,d,n) via
tensor_tensor_scan; n in groups of 4 with batched B/C partition-broadcast
DMAs (double-buffered); y accumulated over n with identity-lhsT PSUM
matmuls, merged into pixel-order ysum straight from PSUM per t-chunk.
"""
import numpy as np
from contextlib import ExitStack

import concourse.bass as bass
import concourse.mybir as mybir

F = mybir.ActivationFunctionType
A = mybir.AluOpType
FP32 = mybir.dt.float32
BF16 = mybir.dt.bfloat16

B_, CIN, CH, COUT, H, W = 4, 96, 96, 96, 48, 48
DI, N, R, K4 = 192, 16, 6, 4
L = H * W               # 2304
HH = 96                 # d-half per core
NT = 3                  # packed (k,d) tiles: 4*96 = 384 = 3*128
HP = 50
LP = 2500
TC = 768                # scan t-chunk (16 rows of 48)
TCH = [(0, 768), (768, 1536), (1536, 2304)]
NG = 4                  # scan n-group (broadcast batch)

# packed (k,d) rows -> (tile j, offset): sections (j, o0, o1, k, d0, d1).
# Section offsets are all 0/32/64 so PE matmuls can write them directly.
SECTIONS = [
    (0, 0, 32, 1, 0, 32),
    (0, 32, 128, 0, 0, 96),
    (1, 0, 64, 1, 32, 96),
    (1, 64, 128, 2, 0, 64),
    (2, 0, 32, 2, 64, 96),
    (2, 32, 128, 3, 0, 96),
]

MM_CHUNKS = [(0, 512), (512, 1024), (1024, 1536), (1536, 2048), (2048, 2304)]
ROW_CHUNKS = [(0, 10), (10, 20), (20, 30), (30, 40), (40, 48)]
SUBS768 = [(0, 512), (512, 768)]
INW_BLOCKS = [(0, 128), (128, 256), (256, 384)]

REPLICA_GROUPS = [[0, 1], [2, 3], [4, 5], [6, 7]]

# ---- const blobs (shared layout between host packing and kernel views) ----
CONSTS_F32 = [
    ('w1T', 96, 96), ('b1', 96, 1), ('linb', 96, 1),
    ('dw1b', 96, 1), ('dw2b', 96, 1),
    ('scb0', 128, 1), ('scb1', 64, 1),
    ('dtb', 128, 3), ('Ap', 128, 48), ('Dsum', 96, 1),
    ('outngA', 96, 1), ('outngB', 96, 1), ('outnbA', 96, 1), ('outnbB', 96, 1),
    ('ag1b', 48, 1), ('ag2b', 96, 1), ('lng', 96, 1), ('lnb', 96, 1),
    ('gwcb', 96, 1), ('finb', 96, 1),
]
CONSTS_BF16 = [
    ('linT', 96, 96),
    ('dw1dg', 96, 864), ('dw2dg', 96, 864),
    ('inwT', 96, 384),
    ('sc0dg', 128, 1152), ('sc1dg', 64, 576),
    ('sel0', 128, 96), ('sel1', 64, 96),
    ('xpTa', 128, 152), ('xpTb', 64, 152),
    ('dtwT', 6, 384),
    ('ident', 128, 128),
    ('outwTa', 96, 96), ('outwTb', 96, 96),
    ('ag1T', 96, 48), ('ag2T', 48, 96),
    ('sq1T', 48, 24), ('sq2T', 48, 24),
    ('gwcT', 24, 864), ('pw1T', 24, 96), ('pw2T', 24, 72),
    ('finT', 96, 96),
]

OFF32 = {}
_o = 0
for _nm, _p, _c in CONSTS_F32:
    OFF32[_nm] = (_o, _p, _c)
    _o += _c
W32 = _o
OFFBF = {}
_o = 0
for _nm, _p, _c in CONSTS_BF16:
    OFFBF[_nm] = (_o, _p, _c)
    _o += _c
WBF = _o


def build_nc():
    nc = bass.Bass(trn_type="TRN2", num_devices=8)

    x_d = nc.dram_tensor("x", [CIN, L], FP32, kind="ExternalInput")
    c32_d = nc.dram_tensor("c32", [128, W32], FP32, kind="ExternalInput")
    cbf_d = nc.dram_tensor("cbf", [128, WBF], BF16, kind="ExternalInput")
    out_d = nc.dram_tensor("out", [COUT, L], FP32, kind="ExternalOutput")

    B_dram = nc.dram_tensor("B_dram", [K4 * N, L], BF16)
    C_dram = nc.dram_tensor("C_dram", [K4 * N, L], BF16)
    y_dram = nc.dram_tensor("y_dram", [HH, L], BF16)
    yg_dram = nc.dram_tensor("yg_dram", [DI, L], BF16)
    st_dram = nc.dram_tensor("st_dram", [2, L], BF16)

    def hw(ap):
        return ap.rearrange("p (h w) -> p h w", h=H)

    def hwp(ap):
        return ap.rearrange("p (h w) -> p h w", h=HP)

    def whv(ap):
        return ap.rearrange("p (h w) -> p w h", h=H)

    with TileContext(nc) as tc:
        glob = ExitStack()
        cst = glob.enter_context(tc.tile_pool(name="cst", bufs=1))
        lngA = glob.enter_context(tc.tile_pool(name="lngA", bufs=1))

        cst32 = cst.tile([128, W32], FP32, tag="cst32")
        cstbf = cst.tile([128, WBF], BF16, tag="cstbf")
        nc.sync.dma_start(cst32[:], c32_d[:])
        nc.sync.dma_start(cstbf[:], cbf_d[:])

        def cvc(nm, a0=0, a1=None, p0=0, p1=None):
            d, tile = (OFF32, cst32) if nm in OFF32 else (OFFBF, cstbf)
            o, p, c = d[nm]
            if a1 is None:
                a1 = c
            if p1 is None:
                p1 = p
            return tile[p0:p1, o + a0:o + a1]

        cv = cvc

        ones96 = cst.tile([HH, 1], BF16, tag="ones96")
        nc.vector.memset(ones96[:], 1.0)

        # long-lived across phases
        z0 = lngA.tile([128, L], BF16, tag="z0")     # z rows 0:128
        z1 = lngA.tile([64, L], BF16, tag="z1")      # z rows 128:192
        x2 = lngA.tile([CH, L], BF16, tag="x2")
        lngB = ExitStack()
        lngB_p = lngB.enter_context(tc.tile_pool(name="lngB_p", bufs=1))
        xch = lngB_p.tile([HH, L], BF16, tag="xch")
        dp = [lngB_p.tile([128, L], BF16, tag=f"dp{j}", name=f"dp{j}") for j in range(NT)]
        # xsp holds packed scan-order xs, overwritten in place with delta*u
        xsp = [lngB_p.tile([128, L], BF16, tag=f"xsp{j}", name=f"xsp{j}") for j in range(NT)]
        ysum = lngB_p.tile([HH, L], BF16, tag="ysum")

        # ================= pre-stage =================
        pre = ExitStack()
        pre_ps = pre.enter_context(tc.tile_pool(name="pre_ps", bufs=4, space="PSUM"))
        pA = pre.enter_context(tc.tile_pool(name="pA", bufs=1))
        pB = pre.enter_context(tc.tile_pool(name="pB", bufs=1))

        xt = pA.tile([CIN, L], FP32, tag="xt")
        nc.sync.dma_start(xt[:], x_d[:])

        # conv1x1 (+folded BN) + ReLU
        h1 = pA.tile([CH, L], BF16, tag="h1")
        for c0, c1 in MM_CHUNKS:
            ps = pre_ps.tile([CH, 512], FP32, tag="ps")
            nc.tensor.matmul(ps[:, :c1 - c0], cv('w1T'), xt[:, c0:c1], start=True, stop=True)
            nc.scalar.activation(h1[:, c0:c1], ps[:, :c1 - c0], F.Relu, bias=cv('b1'))
        # token linear
        h2 = pA.tile([CH, L], BF16, tag="h2")
        for c0, c1 in MM_CHUNKS:
            ps = pre_ps.tile([CH, 512], FP32, tag="ps")
            nc.tensor.matmul(ps[:, :c1 - c0], cv('linT'), h1[:, c0:c1], start=True, stop=True)
            nc.vector.tensor_scalar(out=h2[:, c0:c1], in0=ps[:, :c1 - c0],
                                    scalar1=cv('linb'), scalar2=None, op0=A.add)
        h2p = pA.tile([CH, LP], BF16, tag="h2p")
        nc.gpsimd.memset(h2p[:], 0.0)
        for (r0, r1) in ROW_CHUNKS:
            nc.vector.tensor_copy(hwp(h2p[:])[:, r0 + 1:r1 + 1, 1:49],
                                  hw(h2[:])[:, r0:r1, :])

        def dwconv(dst, src_p, dgname, biasname, nch):
            for (r0, r1) in ROW_CHUNKS:
                nr = r1 - r0
                ps = pre_ps.tile([128, 480], FP32, tag="ps")
                for tap in range(9):
                    dy, dx = tap // 3, tap % 3
                    rhs = hwp(src_p[:])[:, dy + r0:dy + r1, dx:dx + 48]
                    nc.tensor.matmul(ps[:nch, :nr * 48],
                                     cvc(dgname, tap * nch, (tap + 1) * nch),
                                     rhs, start=(tap == 0), stop=(tap == 8))
                nc.scalar.activation(dst[:, r0 * 48:r1 * 48], ps[:nch, :nr * 48],
                                     F.Silu, bias=cv(biasname))

        x1 = pB.tile([CH, L], BF16, tag="x1")
        dwconv(x1, h2p, 'dw1dg', 'dw1b', CH)

        # in-proj: xi (192) + FULL z (192)
        xi0 = pB.tile([128, L], BF16, tag="xi0")
        xi1 = pB.tile([64, L], BF16, tag="xi1")
        for mb, (m0, m1) in enumerate(INW_BLOCKS):
            for c0, c1 in MM_CHUNKS:
                ps = pre_ps.tile([128, 512], FP32, tag="ps")
                nc.tensor.matmul(ps[:m1 - m0, :c1 - c0], cvc('inwT', m0, m1),
                                 x1[:, c0:c1], start=True, stop=True)
                if mb == 0:
                    nc.vector.tensor_copy(xi0[:, c0:c1], ps[:128, :c1 - c0])
                elif mb == 1:
                    nc.scalar.copy(xi1[:, c0:c1], ps[0:64, :c1 - c0])
                    nc.scalar.copy(z0[0:64, c0:c1], ps[64:128, :c1 - c0])
                else:
                    nc.scalar.copy(z0[64:128, c0:c1], ps[0:64, :c1 - c0])
                    nc.scalar.copy(z1[0:64, c0:c1], ps[64:128, :c1 - c0])

        xi0p = pB.tile([128, LP], BF16, tag="xi0p")
        xi1p = pB.tile([64, LP], BF16, tag="xi1p")
        nc.gpsimd.memset(xi0p[:], 0.0)
        nc.gpsimd.memset(xi1p[:], 0.0)
        for (r0, r1) in ROW_CHUNKS:
            nc.vector.tensor_copy(hwp(xi0p[:])[:, r0 + 1:r1 + 1, 1:49],
                                  hw(xi0[:])[:, r0:r1, :])
            nc.vector.tensor_copy(hwp(xi1p[:])[:, r0 + 1:r1 + 1, 1:49],
                                  hw(xi1[:])[:, r0:r1, :])
        xc0 = pB.tile([128, L], BF16, tag="xc0")
        xc1 = pB.tile([64, L], BF16, tag="xc1")
        dwconv(xc0, xi0p, 'sc0dg', 'scb0', 128)
        dwconv(xc1, xi1p, 'sc1dg', 'scb1', 64)

        # d-half extraction + wh copy
        for c0, c1 in MM_CHUNKS:
            ps = pre_ps.tile([HH, 512], FP32, tag="ps")
            nc.tensor.matmul(ps[:, :c1 - c0], cv('sel0'), xc0[:, c0:c1], start=True, stop=False)
            nc.tensor.matmul(ps[:, :c1 - c0], cv('sel1'), xc1[:, c0:c1], start=False, stop=True)
            nc.vector.tensor_copy(xch[:, c0:c1], ps[:, :c1 - c0])
        xwhh = pB.tile([HH, L], BF16, tag="xwhh")
        for (t0, t1) in TCH:
            w0, w1 = t0 // 48, t1 // 48
            nc.vector.tensor_copy(hw(xwhh[:])[:, w0:w1, :],
                                  whv(xch[:])[:, w0:w1, :])

        # xproj (compact 38 rows: 0:6 dts, 6:22 B, 22:38 C) in scan order
        def xc_read(k, c0, c1):
            if k == 0:
                return (xc0[:, c0:c1], xc1[:, c0:c1])
            if k == 1:
                return (whv(xc0[:])[:, c0 // 48:c1 // 48, :],
                        whv(xc1[:])[:, c0 // 48:c1 // 48, :])
            if k == 2:
                return (xc0[:, L - c1:L - c0][:, ::-1],
                        xc1[:, L - c1:L - c0][:, ::-1])
            r0 = whv(xc0[:])[:, (L - c1) // 48:(L - c0) // 48, :][:, ::-1, ::-1]
            r1 = whv(xc1[:])[:, (L - c1) // 48:(L - c0) // 48, :][:, ::-1, ::-1]
            return (r0, r1)

        # row-chunk outer so all 4 directions' early columns finish first;
        # B/C are written to DRAM per scan chunk so ci=0 broadcasts can
        # start while xproj still works on later chunks.
        stage = [pB.tile([38, L], BF16, tag=f"stg{k}", name=f"stg{k}") for k in range(K4)]
        done_w = 0
        for ri, (rr0, rr1) in enumerate(ROW_CHUNKS):
            c0, c1 = rr0 * 48, rr1 * 48
            nf = c1 - c0
            for k in range(K4):
                ra, rb = xc_read(k, c0, c1)
                ps = pre_ps.tile([38, 480], FP32, tag="ps")
                nc.tensor.matmul(ps[:, :nf], cvc('xpTa', k * 38, (k + 1) * 38), ra,
                                 start=True, stop=False)
                nc.tensor.matmul(ps[:, :nf], cvc('xpTb', k * 38, (k + 1) * 38), rb,
                                 start=False, stop=True)
                nc.vector.tensor_copy(stage[k][:, c0:c1], ps[:, :nf])
            while done_w < len(TCH) and TCH[done_w][1] <= c1:
                t0, t1 = TCH[done_w]
                for k in range(K4):
                    nc.sync.dma_start(B_dram[k * N:(k + 1) * N, t0:t1],
                                      stage[k][6:22, t0:t1])
                    nc.sync.dma_start(C_dram[k * N:(k + 1) * N, t0:t1],
                                      stage[k][22:38, t0:t1])
                done_w += 1

        # delta: packed matmuls then softplus on full 128-partition tiles
        def mm_windows(a0, a1):
            if a0 == 0:
                return [(0, a1)]
            res = []
            x = a0
            while x < a1:
                if x % 64 == 32:
                    e = min(a1, x + 32)
                else:  # x == 64
                    e = min(a1, 128)
                res.append((x, e))
                x = e
            return res

        for (cc0, cc1) in MM_CHUNKS:
            cw = cc1 - cc0
            for j in range(NT):
                ex = pre_ps.tile([128, 512], FP32, tag="ps")
                for (jj, o0, o1, k, d0, d1) in SECTIONS:
                    if jj != j:
                        continue
                    for (w0, w1) in mm_windows(o0, o1):
                        dd0 = d0 + (w0 - o0)
                        dd1 = d0 + (w1 - o0)
                        nc.tensor.matmul(ex[w0:w1, :cw],
                                         cvc('dtwT', k * 96 + dd0, k * 96 + dd1),
                                         stage[k][0:6, cc0:cc1], start=True, stop=True)
                # softplus(x+b) = ln(1 + exp(x+b)) (no softplus act table on HW)
                ex2 = pre_ps.tile([128, 512], FP32, tag="ps")
                nc.scalar.activation(ex2[:, :cw], ex[:, :cw], F.Exp,
                                     bias=cvc('dtb', j, j + 1))
                nc.scalar.activation(dp[j][:, cc0:cc1], ex2[:, :cw], F.Ln, bias=1.0)

        # pack scan-order xs (Act copies handle partition shift + flips),
        # then overwrite in place with delta*u = dp*xs.
        # Act partition windows must not cross engine block boundaries on
        # EITHER side: allowed starts 0/32/64/96; a start-32 window may not
        # cross 64. split2 chops a shifted copy accordingly.
        def _legal_span(s):
            return 32 if s == 32 else 128 - s if s else 128

        def split2(o0, i0, ln):
            res = []
            x = 0
            while x < ln:
                step = min(ln - x, _legal_span(o0 + x), _legal_span(i0 + x))
                res.append((x, x + step))
                x += step
            return res

        for (t0, t1) in TCH:
            for (j, o0, o1, k, d0, d1) in SECTIONS:
                v = xwhh if k in (1, 3) else xch
                if k < 2:
                    # forward sections: contiguous rows, cheap DMA shift
                    nc.sync.dma_start(xsp[j][o0:o1, t0:t1], v[d0:d1, t0:t1])
                    continue
                for (w0, w1) in split2(o0, d0, o1 - o0):
                    nc.scalar.copy(xsp[j][o0 + w0:o0 + w1, t0:t1],
                                   v[d0 + w0:d0 + w1, ::-1][:, t0:t1])
            for j in range(NT):
                nc.vector.tensor_mul(xsp[j][:, t0:t1], dp[j][:, t0:t1],
                                     xsp[j][:, t0:t1])

        # branch-2 dwconv last: x2 is only needed in the post stage, so
        # this fills PE/Act slack once the scan inputs are queued
        dwconv(x2, h2p, 'dw2dg', 'dw2b', CH)

        pre.close()

        # ================= scan =================
        sc = ExitStack()
        bbp = sc.enter_context(tc.tile_pool(name="bbp", bufs=2))
        spool = sc.enter_context(tc.tile_pool(name="spool", bufs=2))
        scan_ps = sc.enter_context(tc.tile_pool(name="scan_ps", bufs=1, space="PSUM"))
        stp = sc.enter_context(tc.tile_pool(name="stp", bufs=1))
        state = [stp.tile([128, N], FP32, tag=f"st{j}", name=f"state{j}")
                 for j in range(NT)]
        yd = [stp.tile([HH, L], BF16, tag=f"yd{k}", name=f"yd{k}") for k in range(K4)]

        for ci, (c0, c1) in enumerate(TCH):
            ypsum = [scan_ps.tile([128, TC], FP32, tag=f"yps{j}", name=f"yps{j}_{ci}")
                     for j in range(NT)]
            for g in range(K4):
                Bb = [bbp.tile([128, NG * TC], BF16, tag=f"Bb{j}", name=f"Bb{j}_{ci}_{g}")
                      for j in range(NT)]
                Cb = [bbp.tile([128, NG * TC], BF16, tag=f"Cb{j}", name=f"Cb{j}_{ci}_{g}")
                      for j in range(NT)]
                for (j, o0, o1, k, d0, d1) in SECTIONS:
                    nc.sync.dma_start(
                        Bb[j][o0:o1, :],
                        B_dram[k * N + NG * g:k * N + NG * g + NG, c0:c1]
                        .partition_broadcast(o1 - o0))
                    nc.scalar.dma_start(
                        Cb[j][o0:o1, :],
                        C_dram[k * N + NG * g:k * N + NG * g + NG, c0:c1]
                        .partition_broadcast(o1 - o0))
                for n4 in range(NG):
                    n = NG * g + n4
                    for j in range(NT):
                        at = spool.tile([128, TC], BF16, tag=f"at{j}", name=f"at{j}")
                        nc.scalar.activation(at[:], dp[j][:, c0:c1], F.Exp,
                                             scale=cvc('Ap', j * N + n, j * N + n + 1))
                        bt = spool.tile([128, TC], BF16, tag=f"bt{j}", name=f"bt{j}")
                        nc.vector.tensor_mul(bt[:], xsp[j][:, c0:c1],
                                             Bb[j][:, n4 * TC:(n4 + 1) * TC])
                        ht = spool.tile([128, TC], BF16, tag=f"ht{j}", name=f"ht{j}")
                        if ci > 0:
                            # fold carry state into bt[0] so the scan can use
                            # the cheap zero-init form
                            nc.vector.scalar_tensor_tensor(
                                out=bt[:, 0:1], in0=at[:, 0:1],
                                scalar=state[j][:, n:n + 1], in1=bt[:, 0:1],
                                op0=A.mult, op1=A.add)
                        nc.vector.tensor_tensor_scan(ht[:], at[:], bt[:], 0.0,
                                                     A.mult, A.add)
                        if ci < 2:
                            nc.vector.tensor_copy(state[j][:, n:n + 1], ht[:, TC - 1:TC])
                        gt = spool.tile([128, TC], BF16, tag=f"gt{j}", name=f"gt{j}")
                        nc.vector.tensor_mul(gt[:], ht[:],
                                             Cb[j][:, n4 * TC:(n4 + 1) * TC])
                        for (s0, s1) in SUBS768:
                            nc.tensor.matmul(ypsum[j][:, s0:s1], cv('ident'),
                                             gt[:, s0:s1],
                                             start=(n == 0), stop=(n == N - 1))
            # drain this chunk's PSUM into per-direction scan-order tiles
            # (Act copies allow the partition shift)
            for (j, o0, o1, k, d0, d1) in SECTIONS:
                for (w0, w1) in split2(d0, o0, d1 - d0):
                    nc.scalar.copy(yd[k][d0 + w0:d0 + w1, c0:c1],
                                   ypsum[j][o0 + w0:o0 + w1, :])
        # merge directions into pixel order + D*u term
        tmp96 = stp.tile([HH, L], BF16, tag="tmp96")
        nc.vector.tensor_add(ysum[:], yd[0][:], yd[2][:, ::-1])
        nc.vector.tensor_add(tmp96[:], yd[1][:], yd[3][:, ::-1])
        nc.vector.tensor_add(ysum[:], ysum[:], whv(tmp96[:]))
        nc.vector.scalar_tensor_tensor(out=ysum[:], in0=xch[:], scalar=cv('Dsum'),
                                       in1=ysum[:], op0=A.mult, op1=A.add)
        nc.sync.dma_start(y_dram[:], ysum[:])
        nc.gpsimd.collective_compute(
            "AllGather", A.bypass, replica_groups=REPLICA_GROUPS,
            ins=[y_dram[:]], outs=[yg_dram[:]])
        sc.close()
        lngB.close()

        # ================= post =================
        po = ExitStack()
        post_ps = po.enter_context(tc.tile_pool(name="post_ps", bufs=4, space="PSUM"))
        pP = po.enter_context(tc.tile_pool(name="pP", bufs=1))
        rot = po.enter_context(tc.tile_pool(name="rot", bufs=4))

        # branch 2 + silu(z): no dep on the collective, runs under it
        g1 = rot.tile([48, L], BF16, tag="pb")
        for c0, c1 in MM_CHUNKS:
            ps = post_ps.tile([48, 512], FP32, tag="ps")
            nc.tensor.matmul(ps[:, :c1 - c0], cv('ag1T'), x2[:, c0:c1], start=True, stop=True)
            nc.scalar.activation(g1[:, c0:c1], ps[:, :c1 - c0], F.Relu, bias=cv('ag1b'))
        gat = rot.tile([CH, L], BF16, tag="pb")
        for c0, c1 in MM_CHUNKS:
            ps = post_ps.tile([CH, 512], FP32, tag="ps")
            nc.tensor.matmul(ps[:, :c1 - c0], cv('ag2T'), g1[:, c0:c1], start=True, stop=True)
            nc.scalar.activation(gat[:, c0:c1], ps[:, :c1 - c0], F.Sigmoid, bias=cv('ag2b'))
        x2g = pP.tile([CH, L], BF16, tag="x2g")
        nc.vector.tensor_mul(x2g[:], x2[:], gat[:])

        zsA = pP.tile([HH, L], BF16, tag="zsA")
        zsB = pP.tile([HH, L], BF16, tag="zsB")
        nc.scalar.activation(zsA[:], z0[0:96, :], F.Silu)
        nc.scalar.activation(zsB[0:32, :], z0[96:128, :], F.Silu)
        nc.scalar.activation(zsB[32:64, :], z1[0:32, :], F.Silu)
        nc.scalar.activation(zsB[64:96, :], z1[32:64, :], F.Silu)

        ygA = pP.tile([HH, L], BF16, tag="ygA")
        ygB = pP.tile([HH, L], BF16, tag="ygB")
        nc.sync.dma_start(ygA[:], yg_dram[0:HH, :])
        nc.sync.dma_start(ygB[:], yg_dram[HH:DI, :])

        sA = pP.tile([1, L], FP32, tag="sA")
        sB = pP.tile([1, L], FP32, tag="sB")
        sM = pP.tile([1, L], FP32, tag="sM")

        def ln_stats(cinv, mrb):
            # in: sA=raw sum, sB=raw sumsq; out: mrb [96, 2L] bf16
            # (cols 0:L = mean, L:2L = rstd)
            nc.scalar.activation(sM[:], sA[:], F.Square, scale=cinv)
            nc.vector.scalar_tensor_tensor(out=sB[:], in0=sB[:], scalar=cinv,
                                           in1=sM[:], op0=A.mult, op1=A.subtract)
            nc.vector.tensor_scalar(out=sB[:], in0=sB[:], scalar1=1e-5,
                                    scalar2=None, op0=A.add)
            nc.vector.reciprocal(sB[:], sB[:])
            nc.scalar.activation(sB[:], sB[:], F.Sqrt)
            mbm = pP.tile([1, L], BF16, tag="sbf", bufs=2, name="mbm")
            nc.vector.tensor_scalar(out=mbm[:], in0=sA[:], scalar1=cinv,
                                    scalar2=None, op0=A.mult)
            nc.sync.dma_start(st_dram[0:1, :], mbm[:])
            mbr = pP.tile([1, L], BF16, tag="sbf", bufs=2, name="mbr")
            nc.vector.tensor_copy(mbr[:], sB[:])
            nc.sync.dma_start(st_dram[1:2, :], mbr[:])
            nc.sync.dma_start(mrb[:], st_dram[0:2, :].partition_broadcast(HH))

        # LayerNorm over full DI (local stats via ones-matmul)
        ysqA = rot.tile([HH, L], BF16, tag="pb")
        ysqB = rot.tile([HH, L], BF16, tag="pb")
        nc.vector.tensor_mul(ysqA[:], ygA[:], ygA[:])
        nc.vector.tensor_mul(ysqB[:], ygB[:], ygB[:])
        for c, (c0, c1) in enumerate(MM_CHUNKS):
            ps = post_ps.tile([1, 512], FP32, tag="ps")
            nc.tensor.matmul(ps[:, :c1 - c0], ones96[:], ygA[:, c0:c1], start=True, stop=False)
            nc.tensor.matmul(ps[:, :c1 - c0], ones96[:], ygB[:, c0:c1], start=False, stop=True)
            nc.scalar.copy(sA[0:1, c0:c1], ps[:, :c1 - c0])
            ps2 = post_ps.tile([1, 512], FP32, tag="ps")
            nc.tensor.matmul(ps2[:, :c1 - c0], ones96[:], ysqA[:, c0:c1], start=True, stop=False)
            nc.tensor.matmul(ps2[:, :c1 - c0], ones96[:], ysqB[:, c0:c1], start=False, stop=True)
            nc.scalar.copy(sB[0:1, c0:c1], ps2[:, :c1 - c0])

        mrb1 = pP.tile([HH, 2 * L], BF16, tag="mrb", bufs=2, name="mrb1")
        ln_stats(1.0 / DI, mrb1)

        def apply_ln(dst, src, mrb, gname, bname):
            nc.vector.tensor_sub(dst[:], src[:], mrb[:, 0:L])
            nc.vector.tensor_mul(dst[:], dst[:], mrb[:, L:2 * L])
            nc.vector.tensor_scalar(out=dst[:], in0=dst[:], scalar1=cv(gname),
                                    scalar2=cv(bname), op0=A.mult, op1=A.add)

        ynA = rot.tile([HH, L], BF16, tag="pb")
        ynB = rot.tile([HH, L], BF16, tag="pb")
        apply_ln(ynA, ygA, mrb1, 'outngA', 'outnbA')
        apply_ln(ynB, ygB, mrb1, 'outngB', 'outnbB')

        gzA = rot.tile([HH, L], BF16, tag="pb")
        gzB = rot.tile([HH, L], BF16, tag="pb")
        nc.vector.tensor_mul(gzA[:], ynA[:], zsA[:])
        nc.vector.tensor_mul(gzB[:], ynB[:], zsB[:])

        x1o = pP.tile([CH, L], BF16, tag="x1o")
        for c0, c1 in MM_CHUNKS:
            ps = post_ps.tile([CH, 512], FP32, tag="ps")
            nc.tensor.matmul(ps[:, :c1 - c0], cv('outwTa'), gzA[:, c0:c1],
                             start=True, stop=False)
            nc.tensor.matmul(ps[:, :c1 - c0], cv('outwTb'), gzB[:, c0:c1],
                             start=False, stop=True)
            nc.scalar.copy(x1o[:, c0:c1], ps[:, :c1 - c0])

        yb = pP.tile([CH, L], BF16, tag="yb")
        nc.vector.tensor_add(yb[:], x1o[:], x2g[:])

        # local LayerNorm over channels
        ybsq = rot.tile([CH, L], BF16, tag="pb")
        nc.vector.tensor_mul(ybsq[:], yb[:], yb[:])
        for c, (c0, c1) in enumerate(MM_CHUNKS):
            ps = post_ps.tile([1, 512], FP32, tag="ps")
            nc.tensor.matmul(ps[:, :c1 - c0], ones96[:], yb[:, c0:c1], start=True, stop=True)
            nc.scalar.copy(sA[0:1, c0:c1], ps[:, :c1 - c0])
            ps2 = post_ps.tile([1, 512], FP32, tag="ps")
            nc.tensor.matmul(ps2[:, :c1 - c0], ones96[:], ybsq[:, c0:c1], start=True, stop=True)
            nc.scalar.copy(sB[0:1, c0:c1], ps2[:, :c1 - c0])
        mrb2 = pP.tile([CH, 2 * L], BF16, tag="mrb", bufs=2, name="mrb2")
        ln_stats(1.0 / CH, mrb2)
        ybn = pP.tile([CH, L], BF16, tag="ybn")
        apply_ln(ybn, yb, mrb2, 'lng', 'lnb')

        # CRM
        low_t = rot.tile([48, L], BF16, tag="pb")
        nc.sync.dma_start(low_t[:], ybn[48:96, :])
        upc = pP.tile([24, L], BF16, tag="upc")
        lowc = pP.tile([24, L], BF16, tag="lowc")
        for c0, c1 in MM_CHUNKS:
            ps = post_ps.tile([24, 512], FP32, tag="ps")
            nc.tensor.matmul(ps[:, :c1 - c0], cv('sq1T'), ybn[0:48, c0:c1], start=True, stop=True)
            nc.scalar.copy(upc[:, c0:c1], ps[:, :c1 - c0])
            ps2 = post_ps.tile([24, 512], FP32, tag="ps")
            nc.tensor.matmul(ps2[:, :c1 - c0], cv('sq2T'), low_t[:, c0:c1], start=True, stop=True)
            nc.scalar.copy(lowc[:, c0:c1], ps2[:, :c1 - c0])
        upcp = pP.tile([24, LP], BF16, tag="upcp")
        nc.gpsimd.memset(upcp[:], 0.0)
        nc.vector.tensor_copy(hwp(upcp[:])[:, 1:49, 1:49], hw(upc[:]))
        Y1 = pP.tile([CH, L], BF16, tag="Y1")
        m1c = pP.tile([CH, 5], FP32, tag="m1c")
        for ri, (r0, r1) in enumerate(ROW_CHUNKS):
            nr = r1 - r0
            ps = post_ps.tile([CH, 480], FP32, tag="ps")
            for tap in range(9):
                dy, dx = tap // 3, tap % 3
                rhs = hwp(upcp[:])[:, dy + r0:dy + r1, dx:dx + 48]
                nc.tensor.matmul(ps[:, :nr * 48], cvc('gwcT', tap * CH, (tap + 1) * CH),
                                 rhs, start=(tap == 0), stop=False)
            nc.tensor.matmul(ps[:, :nr * 48], cv('pw1T'), upc[:, r0 * 48:r1 * 48],
                             start=False, stop=True)
            nc.scalar.activation(Y1[:, r0 * 48:r1 * 48], ps[:, :nr * 48],
                                 F.Identity, bias=cv('gwcb'),
                                 accum_out=m1c[:, ri:ri + 1])
        Y2a = pP.tile([72, L], BF16, tag="Y2a")
        m2ca = pP.tile([72, 5], FP32, tag="m2ca")
        m2cb = pP.tile([24, 5], FP32, tag="m2cb")
        for ri, (c0, c1) in enumerate(MM_CHUNKS):
            ps = post_ps.tile([72, 512], FP32, tag="ps")
            nc.tensor.matmul(ps[:, :c1 - c0], cv('pw2T'), lowc[:, c0:c1], start=True, stop=True)
            nc.scalar.activation(Y2a[:, c0:c1], ps[:, :c1 - c0], F.Identity,
                                 accum_out=m2ca[:, ri:ri + 1])
            scr = post_ps.tile([24, 512], FP32, tag="ps")
            nc.scalar.activation(scr[:, :c1 - c0], lowc[:, c0:c1], F.Identity,
                                 accum_out=m2cb[:, ri:ri + 1])
        m1 = pP.tile([CH, 1], FP32, tag="m1")
        m2a_s = pP.tile([72, 1], FP32, tag="m2a_s")
        m2b_s = pP.tile([24, 1], FP32, tag="m2b_s")
        nc.vector.reduce_sum(m1[:], m1c[:], axis=mybir.AxisListType.X)
        nc.vector.reduce_sum(m2a_s[:], m2ca[:], axis=mybir.AxisListType.X)
        nc.vector.reduce_sum(m2b_s[:], m2cb[:], axis=mybir.AxisListType.X)
        smf = pP.tile([1, 2 * CH], FP32, tag="smf")
        nc.sync.dma_start(smf[0:1, 0:CH], m1[:, 0:1])
        nc.sync.dma_start(smf[0:1, CH:CH + 72], m2a_s[:, 0:1])
        nc.sync.dma_start(smf[0:1, CH + 72:2 * CH], m2b_s[:, 0:1])
        nc.vector.tensor_scalar(out=smf[:], in0=smf[:], scalar1=1.0 / L,
                                scalar2=None, op0=A.mult)
        mx = pP.tile([1, 1], FP32, tag="mx")
        nc.vector.reduce_max(mx[:], smf[:], axis=mybir.AxisListType.X)
        nc.vector.tensor_scalar(out=mx[:], in0=mx[:], scalar1=-1.0,
                                scalar2=None, op0=A.mult)
        nc.scalar.activation(smf[:], smf[:], F.Exp, bias=mx[0:1, 0:1])
        sm_s = pP.tile([1, 1], FP32, tag="sm_s")
        nc.vector.reduce_sum(sm_s[:], smf[:], axis=mybir.AxisListType.X)
        nc.vector.reciprocal(sm_s[:], sm_s[:])
        nc.vector.tensor_scalar(out=smf[:], in0=smf[:], scalar1=sm_s[0:1, 0:1],
                                scalar2=None, op0=A.mult)
        sm1 = pP.tile([CH, 1], FP32, tag="sm1")
        sm2 = pP.tile([CH, 1], FP32, tag="sm2")
        nc.sync.dma_start(sm1[:, 0:1], smf[0:1, 0:CH])
        nc.sync.dma_start(sm2[:, 0:1], smf[0:1, CH:2 * CH])
        o2f = rot.tile([CH, L], BF16, tag="pb")
        nc.sync.dma_start(o2f[0:72, :], Y2a[:])
        nc.sync.dma_start(o2f[72:96, :], lowc[:])
        o2t = pP.tile([CH, L], BF16, tag="o2t")
        nc.vector.tensor_scalar(out=o2t[:], in0=o2f[:], scalar1=sm2[:, 0:1],
                                scalar2=None, op0=A.mult)
        yc = pP.tile([CH, L], BF16, tag="yc")
        nc.vector.scalar_tensor_tensor(out=yc[:], in0=Y1[:], scalar=sm1[:, 0:1],
                                       in1=o2t[:], op0=A.mult, op1=A.add)
        outt = pP.tile([COUT, L], FP32, tag="outt")
        for c0, c1 in MM_CHUNKS:
            ps = post_ps.tile([COUT, 512], FP32, tag="ps")
            nc.tensor.matmul(ps[:, :c1 - c0], cv('finT'), yc[:, c0:c1], start=True, stop=True)
            nc.scalar.activation(outt[:, c0:c1], ps[:, :c1 - c0], F.Identity, bias=cv('finb'))
        nc.sync.dma_start(out_d[:], outt[:])
        po.close()
        glob.close()
    split_multi_waits(nc, max_waits=1)
    return nc


# =============================== host side ==================================

def prep_core_inputs(inputs, b, half):
    import ml_dtypes
    f32 = np.float32
    bf16 = ml_dtypes.bfloat16
    d0 = half * HH

    def asf(a):
        return np.asarray(a, f32)

    bnscale = asf(inputs['bn_g']) / np.sqrt(np.float32(1.0 + 1e-5))
    w1 = asf(inputs['conv1_w'])[:, :, 0, 0] * bnscale[:, None]
    b1 = asf(inputs['conv1_b']) * bnscale + asf(inputs['bn_b'])

    def diag9(w, nch):
        out = np.zeros((nch, 9 * nch), f32)
        w = asf(w)
        for tap in range(9):
            dy, dx = tap // 3, tap % 3
            blk = out[:, tap * nch:(tap + 1) * nch]
            np.fill_diagonal(blk, w[:, 0, dy, dx])
        return out

    sscd = diag9(inputs['ss_conv_w'], DI)        # (192, 9*192)
    sc0 = np.zeros((128, 9 * 128), f32)
    sc1 = np.zeros((64, 9 * 64), f32)
    for tap in range(9):
        blk = sscd[:, tap * DI:(tap + 1) * DI]
        sc0[:, tap * 128:(tap + 1) * 128] = blk[0:128, 0:128]
        sc1[:, tap * 64:(tap + 1) * 64] = blk[128:192, 128:192]

    sel = np.zeros((DI, HH), f32)
    sel[np.arange(d0, d0 + HH), np.arange(HH)] = 1.0

    xp = asf(inputs['ss_xproj_w'])               # (4, 38, 192)
    xpTa = np.zeros((128, K4 * 38), f32)
    xpTb = np.zeros((64, K4 * 38), f32)
    for k in range(K4):
        xpT = xp[k].T                            # (192, 38)
        xpTa[:, k * 38:(k + 1) * 38] = xpT[0:128]
        xpTb[:, k * 38:(k + 1) * 38] = xpT[128:192]

    dtw = asf(inputs['ss_dt_w'])
    dtwT = np.zeros((R, K4 * HH), f32)
    for k in range(K4):
        dtwT[:, k * HH:(k + 1) * HH] = dtw[k][d0:d0 + HH, :].T

    dtb_full = asf(inputs['ss_dt_b'])
    Alog = asf(inputs['ss_Alog']).reshape(K4, DI, N)
    Dv = asf(inputs['ss_D']).reshape(K4, DI)
    dtb_p = np.zeros((128, NT), f32)
    Ap = np.zeros((128, NT * N), f32)
    for (j, o0, o1, k, dd0, dd1) in SECTIONS:
        dtb_p[o0:o1, j] = dtb_full[k, d0 + dd0:d0 + dd1]
        Ap[o0:o1, j * N:(j + 1) * N] = -np.exp(Alog[k, d0 + dd0:d0 + dd1])
    Dsum = Dv[:, d0:d0 + HH].sum(0)[:, None]

    gw = asf(inputs['gwc_w'])
    gT = np.zeros((24, 9 * CH), f32)
    for tap in range(9):
        dy, dx = tap // 3, tap % 3
        blk = np.zeros((24, CH), f32)
        blk[0:12, 0:48] = gw[0:48, :, dy, dx].T
        blk[12:24, 48:96] = gw[48:96, :, dy, dx].T
        gT[:, tap * CH:(tap + 1) * CH] = blk

    owT = asf(inputs['ss_out_w']).T              # (192, 96)
    outn_g = asf(inputs['ss_outn_g'])
    outn_b = asf(inputs['ss_outn_b'])

    vals32 = {
        'w1T': w1.T, 'b1': b1[:, None],
        'linb': asf(inputs['lin_b'])[:, None],
        'dw1b': asf(inputs['dw1_b'])[:, None],
        'dw2b': asf(inputs['dw2_b'])[:, None],
        'scb0': asf(inputs['ss_conv_b'])[0:128, None],
        'scb1': asf(inputs['ss_conv_b'])[128:192, None],
        'dtb': dtb_p, 'Ap': Ap, 'Dsum': Dsum,
        'outngA': outn_g[0:96, None], 'outngB': outn_g[96:192, None],
        'outnbA': outn_b[0:96, None], 'outnbB': outn_b[96:192, None],
        'ag1b': asf(inputs['ag1_b'])[:, None],
        'ag2b': asf(inputs['ag2_b'])[:, None],
        'lng': asf(inputs['ln_g'])[:, None],
        'lnb': asf(inputs['ln_b'])[:, None],
        'gwcb': asf(inputs['gwc_b'])[:, None],
        'finb': asf(inputs['fin_b'])[:, None],
    }
    valsbf = {
        'linT': asf(inputs['lin_w']).T,
        'dw1dg': diag9(inputs['dw1_w'], CH),
        'dw2dg': diag9(inputs['dw2_w'], CH),
        'inwT': asf(inputs['ss_in_w']).T,        # (96, 384) full z
        'sc0dg': sc0, 'sc1dg': sc1,
        'sel0': sel[0:128], 'sel1': sel[128:192],
        'xpTa': xpTa, 'xpTb': xpTb,
        'dtwT': dtwT,
        'ident': np.eye(128, dtype=f32),
        'outwTa': owT[0:96], 'outwTb': owT[96:192],
        'ag1T': asf(inputs['ag1_w'])[:, :, 0, 0].T,
        'ag2T': asf(inputs['ag2_w'])[:, :, 0, 0].T,
        'sq1T': asf(inputs['sq1_w'])[:, :, 0, 0].T,
        'sq2T': asf(inputs['sq2_w'])[:, :, 0, 0].T,
        'gwcT': gT,
        'pw1T': asf(inputs['pwc1_w'])[:, :, 0, 0].T,
        'pw2T': asf(inputs['pwc2_w'])[:, :, 0, 0].T,
        'finT': asf(inputs['fin_w']).T,
    }

    blob32 = np.zeros((128, W32), f32)
    for nm, p, c in CONSTS_F32:
        o = OFF32[nm][0]
        v = vals32[nm]
        assert v.shape == (p, c), (nm, v.shape, (p, c))
        blob32[0:p, o:o + c] = v
    blobbf = np.zeros((128, WBF), bf16)
    for nm, p, c in CONSTS_BF16:
        o = OFFBF[nm][0]
        v = valsbf[nm]
        assert v.shape == (p, c), (nm, v.shape, (p, c))
        blobbf[0:p, o:o + c] = v.astype(bf16)

    return {
        'x': np.ascontiguousarray(asf(inputs['x'])[b].reshape(CIN, L)),
        'c32': blob32,
        'cbf': np.ascontiguousarray(blobbf),
    }


_NC_CACHE = {}


def get_nc():
    if 'nc' not in _NC_CACHE:
        _NC_CACHE['nc'] = build_nc()
    return _NC_CACHE['nc']


def kernel(**inputs):
    from concourse.bass_utils import run_bass_kernel_spmd
    nc = get_nc()
    in_maps = [prep_core_inputs(inputs, c // 2, c % 2) for c in range(8)]
    res = run_bass_kernel_spmd(nc, in_maps, core_ids=list(range(8)))
    out = np.zeros((B_, COUT, H, W), np.float32)
    for b in range(B_):
        out[b] = res.results[2 * b]['out'].reshape(COUT, H, W)
    return out


# revision 54
# speedup vs baseline: 1.0052x; 1.0052x over previous
"""Self-contained Trainium2 Bass kernel for the CR-VSS block (8 cores)."""

# ---- TileContext drain-wait patch (walrus 1-wait limit) ----
"""Patch TileContext._drain_and_barrier: the axon-client walrus rejects
instructions carrying >2 sem waits ("Too many sync wait commands" in
setupSyncWait for CTRL structs). Redistribute the exit-drain's waits across
preceding SP nop instructions, each carrying at most MAX_WAITS."""
from concourse.tile import TileContext, ScopedClock

MAX_WAITS = 1


def _patched_drain_and_barrier(self, tick_clock, wait_clock):
    nc = self.nc
    drain_inst = nc.sync.drain()
    wait_clock.add_sem_waits(
        drain_inst.ins, ScopedClock({None: tick_clock.global_clock})
    )

    waits = list(drain_inst.ins.sync_info.on_wait or [])
    if len(waits) > MAX_WAITS:
        bb = nc.cur_bb.bb
        assert bb.instructions[-1] is drain_inst.ins
        # strip waits from the drain, re-emit them on nop carriers
        drain_inst.ins.sync_info.on_wait = waits[:0]
        carriers = []
        import concourse.mybir as mybir
        for i in range(0, len(waits), MAX_WAITS):
            nop = nc.sync.nop(nofuse=True)
            nop.ins.sync_info = mybir.SyncInfo(
                on_wait=waits[i:i + MAX_WAITS], on_update=[]
            )
            carriers.append(nop.ins)
        # move carriers before the drain
        insts = list(bb.instructions)
        assert insts[-len(carriers) - 1] is drain_inst.ins
        reordered = insts[:-len(carriers) - 1] + insts[-len(carriers):] + [drain_inst.ins]
        while len(bb.instructions):
            bb.instructions.pop()
        for x in reordered:
            bb.instructions.append(x)

    nc.all_engine_barrier()
    assert self.sems is not None
    popped = nc._tile_sem_poison_stack.pop()
    assert popped is self._sem_poison
    nc.clear_and_free_semaphores(list(self.sems.allocated().values()))
    nc.all_engine_barrier()


def apply():
    TileContext._drain_and_barrier = _patched_drain_and_barrier


def split_multi_waits(nc, max_waits=1):
    """Post-pass: walrus CTRL codegen rejects instructions with more than
    one sem wait. Move extra waits onto same-engine NoOp carriers."""
    import concourse.mybir as mybir
    for f in nc.m.functions:
        for bb in f.blocks:
            insts = list(bb.instructions)
            out = []
            changed = False
            for ins in insts:
                si = ins.sync_info
                if si is not None and si.on_wait and len(si.on_wait) > max_waits:
                    waits = list(si.on_wait)
                    for i, w in enumerate(waits[max_waits:]):
                        nop = mybir.InstNoOp.__new__(
                            mybir.InstNoOp, name=f"{ins.name}-xw{i}", ins=[], outs=[])
                        nop.engine = ins.engine
                        nop.sync_info = mybir.SyncInfo(on_wait=[w], on_update=[])
                        out.append(nop)
                    ins.sync_info = mybir.SyncInfo(
                        on_wait=waits[:max_waits],
                        on_update=list(si.on_update or []))
                    changed = True
                out.append(ins)
            if changed:
                while len(bb.instructions):
                    bb.instructions.pop()
                for x in out:
                    bb.instructions.append(x)

apply()

# ---- kernel ----
"""Trainium2 Bass kernel for nn_CR_VSS (VSS block with SS2D selective scan).

Sharding: 8 cores = 4 samples x 2 d_inner-halves. Each core runs the full
pre-stage for its sample, scans its 96-channel d-half across all 4
cross-scan directions (packed into 3x128-partition tiles), then the pair
exchanges y-halves with ONE AllGather; LN + out-proj + post-stage run
locally (z is computed full-width in the in-proj so no second collective).

Scan: h_t = exp(A*delta_t)*h_{t-1} + delta_t*u_t*B_t per (k,d,n) via
tensor_tensor_scan; n in groups of 4 with batched B/C partition-broadcast
DMAs (double-buffered); y accumulated over n with identity-lhsT PSUM
matmuls, merged into pixel-order ysum straight from PSUM per t-chunk.
"""
import numpy as np
from contextlib import ExitStack

import concourse.bass as bass
import concourse.mybir as mybir

F = mybir.ActivationFunctionType
A = mybir.AluOpType
FP32 = mybir.dt.float32
BF16 = mybir.dt.bfloat16

B_, CIN, CH, COUT, H, W = 4, 96, 96, 96, 48, 48
DI, N, R, K4 = 192, 16, 6, 4
L = H * W               # 2304
HH = 96                 # d-half per core
NT = 3                  # packed (k,d) tiles: 4*96 = 384 = 3*128
HP = 50
LP = 2500
TC = 768                # scan t-chunk (16 rows of 48)
TCH = [(0, 768), (768, 1536), (1536, 2304)]
NG = 4                  # scan n-group (broadcast batch)

# packed (k,d) rows -> (tile j, offset): sections (j, o0, o1, k, d0, d1).
# Section offsets are all 0/32/64 so PE matmuls can write them directly.
SECTIONS = [
    (0, 0, 32, 1, 0, 32),
    (0, 32, 128, 0, 0, 96),
    (1, 0, 64, 1, 32, 96),
    (1, 64, 128, 2, 0, 64),
    (2, 0, 32, 2, 64, 96),
    (2, 32, 128, 3, 0, 96),
]

MM_CHUNKS = [(0, 512), (512, 1024), (1024, 1536), (1536, 2048), (2048, 2304)]
ROW_CHUNKS = [(0, 10), (10, 20), (20, 30), (30, 40), (40, 48)]
SUBS768 = [(0, 512), (512, 768)]
INW_BLOCKS = [(0, 128), (128, 256), (256, 384)]

REPLICA_GROUPS = [[0, 1], [2, 3], [4, 5], [6, 7]]

# ---- const blobs (shared layout between host packing and kernel views) ----
CONSTS_F32 = [
    ('w1T', 96, 96), ('b1', 96, 1), ('linb', 96, 1),
    ('dw1b', 96, 1), ('dw2b', 96, 1),
    ('scb0', 128, 1), ('scb1', 64, 1),
    ('dtb', 128, 3), ('Ap', 128, 48), ('Dsum', 96, 1),
    ('outngA', 96, 1), ('outngB', 96, 1), ('outnbA', 96, 1), ('outnbB', 96, 1),
    ('ag1b', 48, 1), ('ag2b', 96, 1), ('lng', 96, 1), ('lnb', 96, 1),
    ('gwcb', 96, 1), ('finb', 96, 1), ('bcv', 1, 288),
]
CONSTS_BF16 = [
    ('linT', 96, 96),
    ('dw1dg', 96, 864), ('dw2dg', 96, 864),
    ('inwT', 96, 384),
    ('sc0dg', 128, 1152), ('sc1dg', 64, 576),
    ('sel0', 128, 96), ('sel1', 64, 96),
    ('xpTa', 128, 152), ('xpTb', 64, 152),
    ('dtwT', 6, 384),
    ('ident', 128, 128),
    ('outwTa', 96, 96), ('outwTb', 96, 96),
    ('ag1T', 96, 48), ('ag2T', 48, 96),
    ('sq1T', 48, 24), ('sq2T', 48, 24),
    ('gwcT', 24, 864), ('pw1T', 24, 96), ('pw2T', 24, 72),
    ('finT', 96, 96),
]

OFF32 = {}
_o = 0
for _nm, _p, _c in CONSTS_F32:
    OFF32[_nm] = (_o, _p, _c)
    _o += _c
W32 = _o
OFFBF = {}
_o = 0
for _nm, _p, _c in CONSTS_BF16:
    OFFBF[_nm] = (_o, _p, _c)
    _o += _c
WBF = _o


def build_nc():
    nc = bass.Bass(trn_type="TRN2", num_devices=8)

    x_d = nc.dram_tensor("x", [CIN, L], FP32, kind="ExternalInput")
    c32_d = nc.dram_tensor("c32", [128, W32], FP32, kind="ExternalInput")
    cbf_d = nc.dram_tensor("cbf", [128, WBF], BF16, kind="ExternalInput")
    out_d = nc.dram_tensor("out", [COUT, L], FP32, kind="ExternalOutput")

    B_dram = nc.dram_tensor("B_dram", [K4 * N, L], BF16)
    C_dram = nc.dram_tensor("C_dram", [K4 * N, L], BF16)
    y_dram = nc.dram_tensor("y_dram", [HH, L], BF16)
    yg_dram = nc.dram_tensor("yg_dram", [DI, L], BF16)
    st_dram = nc.dram_tensor("st_dram", [2, L], BF16)

    def hw(ap):
        return ap.rearrange("p (h w) -> p h w", h=H)

    def hwp(ap):
        return ap.rearrange("p (h w) -> p h w", h=HP)

    def whv(ap):
        return ap.rearrange("p (h w) -> p w h", h=H)

    with TileContext(nc) as tc:
        glob = ExitStack()
        cst = glob.enter_context(tc.tile_pool(name="cst", bufs=1))
        lngA = glob.enter_context(tc.tile_pool(name="lngA", bufs=1))

        cst32 = cst.tile([128, W32], FP32, tag="cst32")
        cstbf = cst.tile([128, WBF], BF16, tag="cstbf")
        nc.sync.dma_start(cst32[:], c32_d[:])
        nc.sync.dma_start(cstbf[:], cbf_d[:])

        def cvc(nm, a0=0, a1=None, p0=0, p1=None):
            d, tile = (OFF32, cst32) if nm in OFF32 else (OFFBF, cstbf)
            o, p, c = d[nm]
            if a1 is None:
                a1 = c
            if p1 is None:
                p1 = p
            return tile[p0:p1, o + a0:o + a1]

        cv = cvc

        ones96 = cst.tile([HH, 1], BF16, tag="ones96")
        nc.vector.memset(ones96[:], 1.0)

        # long-lived across phases
        z0 = lngA.tile([128, L], BF16, tag="z0")     # z rows 0:128
        z1 = lngA.tile([64, L], BF16, tag="z1")      # z rows 128:192
        x2 = lngA.tile([CH, L], BF16, tag="x2")
        lngB = ExitStack()
        lngB_p = lngB.enter_context(tc.tile_pool(name="lngB_p", bufs=1))
        xch = lngB_p.tile([HH, L], BF16, tag="xch")
        dp = [lngB_p.tile([128, L], BF16, tag=f"dp{j}", name=f"dp{j}") for j in range(NT)]
        # xsp holds packed scan-order xs, overwritten in place with delta*u
        xsp = [lngB_p.tile([128, L], BF16, tag=f"xsp{j}", name=f"xsp{j}") for j in range(NT)]
        ysum = lngB_p.tile([HH, L], BF16, tag="ysum")

        # ================= pre-stage =================
        pre = ExitStack()
        pre_ps = pre.enter_context(tc.tile_pool(name="pre_ps", bufs=4, space="PSUM"))
        pA = pre.enter_context(tc.tile_pool(name="pA", bufs=1))
        pB = pre.enter_context(tc.tile_pool(name="pB", bufs=1))

        xt = pA.tile([CIN, L], FP32, tag="xt")
        nc.sync.dma_start(xt[:], x_d[:])

        # conv1x1 (+folded BN) + ReLU
        h1 = pA.tile([CH, L], BF16, tag="h1")
        for c0, c1 in MM_CHUNKS:
            ps = pre_ps.tile([CH, 512], FP32, tag="ps")
            nc.tensor.matmul(ps[:, :c1 - c0], cv('w1T'), xt[:, c0:c1], start=True, stop=True)
            nc.scalar.activation(h1[:, c0:c1], ps[:, :c1 - c0], F.Relu, bias=cv('b1'))
        # token linear
        h2 = pA.tile([CH, L], BF16, tag="h2")
        for c0, c1 in MM_CHUNKS:
            ps = pre_ps.tile([CH, 512], FP32, tag="ps")
            nc.tensor.matmul(ps[:, :c1 - c0], cv('linT'), h1[:, c0:c1], start=True, stop=True)
            nc.vector.tensor_scalar(out=h2[:, c0:c1], in0=ps[:, :c1 - c0],
                                    scalar1=cv('linb'), scalar2=None, op0=A.add)
        h2p = pA.tile([CH, LP], BF16, tag="h2p")
        nc.gpsimd.memset(h2p[:], 0.0)
        for (r0, r1) in ROW_CHUNKS:
            nc.vector.tensor_copy(hwp(h2p[:])[:, r0 + 1:r1 + 1, 1:49],
                                  hw(h2[:])[:, r0:r1, :])

        def dwconv(dst, src_p, dgname, biasname, nch):
            for (r0, r1) in ROW_CHUNKS:
                nr = r1 - r0
                ps = pre_ps.tile([128, 480], FP32, tag="ps")
                for tap in range(9):
                    dy, dx = tap // 3, tap % 3
                    rhs = hwp(src_p[:])[:, dy + r0:dy + r1, dx:dx + 48]
                    nc.tensor.matmul(ps[:nch, :nr * 48],
                                     cvc(dgname, tap * nch, (tap + 1) * nch),
                                     rhs, start=(tap == 0), stop=(tap == 8))
                nc.scalar.activation(dst[:, r0 * 48:r1 * 48], ps[:nch, :nr * 48],
                                     F.Silu, bias=cv(biasname))

        x1 = pB.tile([CH, L], BF16, tag="x1")
        dwconv(x1, h2p, 'dw1dg', 'dw1b', CH)

        # in-proj: xi (192) + FULL z (192)
        xi0 = pB.tile([128, L], BF16, tag="xi0")
        xi1 = pB.tile([64, L], BF16, tag="xi1")
        for mb, (m0, m1) in enumerate(INW_BLOCKS):
            for c0, c1 in MM_CHUNKS:
                ps = pre_ps.tile([128, 512], FP32, tag="ps")
                nc.tensor.matmul(ps[:m1 - m0, :c1 - c0], cvc('inwT', m0, m1),
                                 x1[:, c0:c1], start=True, stop=True)
                if mb == 0:
                    nc.vector.tensor_copy(xi0[:, c0:c1], ps[:128, :c1 - c0])
                elif mb == 1:
                    nc.scalar.copy(xi1[:, c0:c1], ps[0:64, :c1 - c0])
                    nc.scalar.copy(z0[0:64, c0:c1], ps[64:128, :c1 - c0])
                else:
                    nc.scalar.copy(z0[64:128, c0:c1], ps[0:64, :c1 - c0])
                    nc.scalar.copy(z1[0:64, c0:c1], ps[64:128, :c1 - c0])

        xi0p = pB.tile([128, LP], BF16, tag="xi0p")
        xi1p = pB.tile([64, LP], BF16, tag="xi1p")
        nc.gpsimd.memset(xi0p[:], 0.0)
        nc.gpsimd.memset(xi1p[:], 0.0)
        for (r0, r1) in ROW_CHUNKS:
            nc.vector.tensor_copy(hwp(xi0p[:])[:, r0 + 1:r1 + 1, 1:49],
                                  hw(xi0[:])[:, r0:r1, :])
            nc.vector.tensor_copy(hwp(xi1p[:])[:, r0 + 1:r1 + 1, 1:49],
                                  hw(xi1[:])[:, r0:r1, :])
        xc0 = pB.tile([128, L], BF16, tag="xc0")
        xc1 = pB.tile([64, L], BF16, tag="xc1")
        dwconv(xc0, xi0p, 'sc0dg', 'scb0', 128)
        dwconv(xc1, xi1p, 'sc1dg', 'scb1', 64)

        # d-half extraction + wh copy
        for c0, c1 in MM_CHUNKS:
            ps = pre_ps.tile([HH, 512], FP32, tag="ps")
            nc.tensor.matmul(ps[:, :c1 - c0], cv('sel0'), xc0[:, c0:c1], start=True, stop=False)
            nc.tensor.matmul(ps[:, :c1 - c0], cv('sel1'), xc1[:, c0:c1], start=False, stop=True)
            nc.vector.tensor_copy(xch[:, c0:c1], ps[:, :c1 - c0])
        xwhh = pB.tile([HH, L], BF16, tag="xwhh")
        for (t0, t1) in TCH:
            w0, w1 = t0 // 48, t1 // 48
            nc.vector.tensor_copy(hw(xwhh[:])[:, w0:w1, :],
                                  whv(xch[:])[:, w0:w1, :])

        # xproj (compact 38 rows: 0:6 dts, 6:22 B, 22:38 C) in scan order
        def xc_read(k, c0, c1):
            if k == 0:
                return (xc0[:, c0:c1], xc1[:, c0:c1])
            if k == 1:
                return (whv(xc0[:])[:, c0 // 48:c1 // 48, :],
                        whv(xc1[:])[:, c0 // 48:c1 // 48, :])
            if k == 2:
                return (xc0[:, L - c1:L - c0][:, ::-1],
                        xc1[:, L - c1:L - c0][:, ::-1])
            r0 = whv(xc0[:])[:, (L - c1) // 48:(L - c0) // 48, :][:, ::-1, ::-1]
            r1 = whv(xc1[:])[:, (L - c1) // 48:(L - c0) // 48, :][:, ::-1, ::-1]
            return (r0, r1)

        # row-chunk outer so all 4 directions' early columns finish first;
        # B/C are written to DRAM per scan chunk so ci=0 broadcasts can
        # start while xproj still works on later chunks.
        stage = [pB.tile([38, L], BF16, tag=f"stg{k}", name=f"stg{k}") for k in range(K4)]
        done_w = 0
        for ri, (rr0, rr1) in enumerate(ROW_CHUNKS):
            c0, c1 = rr0 * 48, rr1 * 48
            nf = c1 - c0
            for k in range(K4):
                ra, rb = xc_read(k, c0, c1)
                ps = pre_ps.tile([38, 480], FP32, tag="ps")
                nc.tensor.matmul(ps[:, :nf], cvc('xpTa', k * 38, (k + 1) * 38), ra,
                                 start=True, stop=False)
                nc.tensor.matmul(ps[:, :nf], cvc('xpTb', k * 38, (k + 1) * 38), rb,
                                 start=False, stop=True)
                nc.vector.tensor_copy(stage[k][:, c0:c1], ps[:, :nf])
            while done_w < len(TCH) and TCH[done_w][1] <= c1:
                t0, t1 = TCH[done_w]
                for k in range(K4):
                    nc.sync.dma_start(B_dram[k * N:(k + 1) * N, t0:t1],
                                      stage[k][6:22, t0:t1])
                    nc.sync.dma_start(C_dram[k * N:(k + 1) * N, t0:t1],
                                      stage[k][22:38, t0:t1])
                done_w += 1

        # delta: packed matmuls then softplus on full 128-partition tiles
        def mm_windows(a0, a1):
            if a0 == 0:
                return [(0, a1)]
            res = []
            x = a0
            while x < a1:
                if x % 64 == 32:
                    e = min(a1, x + 32)
                else:  # x == 64
                    e = min(a1, 128)
                res.append((x, e))
                x = e
            return res

        for (cc0, cc1) in MM_CHUNKS:
            cw = cc1 - cc0
            for j in range(NT):
                ex = pre_ps.tile([128, 512], FP32, tag="ps")
                for (jj, o0, o1, k, d0, d1) in SECTIONS:
                    if jj != j:
                        continue
                    for (w0, w1) in mm_windows(o0, o1):
                        dd0 = d0 + (w0 - o0)
                        dd1 = d0 + (w1 - o0)
                        nc.tensor.matmul(ex[w0:w1, :cw],
                                         cvc('dtwT', k * 96 + dd0, k * 96 + dd1),
                                         stage[k][0:6, cc0:cc1], start=True, stop=True)
                # softplus(x+b) = ln(1 + exp(x+b)) (no softplus act table on HW)
                ex2 = pre_ps.tile([128, 512], FP32, tag="ps")
                nc.scalar.activation(ex2[:, :cw], ex[:, :cw], F.Exp,
                                     bias=cvc('dtb', j, j + 1))
                nc.scalar.activation(dp[j][:, cc0:cc1], ex2[:, :cw], F.Ln, bias=1.0)

        # pack scan-order xs (Act copies handle partition shift + flips),
        # then overwrite in place with delta*u = dp*xs.
        # Act partition windows must not cross engine block boundaries on
        # EITHER side: allowed starts 0/32/64/96; a start-32 window may not
        # cross 64. split2 chops a shifted copy accordingly.
        def _legal_span(s):
            return 32 if s == 32 else 128 - s if s else 128

        def split2(o0, i0, ln):
            res = []
            x = 0
            while x < ln:
                step = min(ln - x, _legal_span(o0 + x), _legal_span(i0 + x))
                res.append((x, x + step))
                x += step
            return res

        for (t0, t1) in TCH:
            for (j, o0, o1, k, d0, d1) in SECTIONS:
                v = xwhh if k in (1, 3) else xch
                if k < 2:
                    # forward sections: contiguous rows, cheap DMA shift
                    nc.sync.dma_start(xsp[j][o0:o1, t0:t1], v[d0:d1, t0:t1])
                    continue
                for (w0, w1) in split2(o0, d0, o1 - o0):
                    nc.scalar.copy(xsp[j][o0 + w0:o0 + w1, t0:t1],
                                   v[d0 + w0:d0 + w1, ::-1][:, t0:t1])
            for j in range(NT):
                nc.vector.tensor_mul(xsp[j][:, t0:t1], dp[j][:, t0:t1],
                                     xsp[j][:, t0:t1])

        # branch-2 dwconv last: x2 is only needed in the post stage, so
        # this fills PE/Act slack once the scan inputs are queued
        dwconv(x2, h2p, 'dw2dg', 'dw2b', CH)

        pre.close()

        # ================= scan =================
        sc = ExitStack()
        bbp = sc.enter_context(tc.tile_pool(name="bbp", bufs=2))
        spool = sc.enter_context(tc.tile_pool(name="spool", bufs=2))
        scan_ps = sc.enter_context(tc.tile_pool(name="scan_ps", bufs=1, space="PSUM"))
        stp = sc.enter_context(tc.tile_pool(name="stp", bufs=1))
        state = [stp.tile([128, N], FP32, tag=f"st{j}", name=f"state{j}")
                 for j in range(NT)]
        yd = [stp.tile([HH, L], BF16, tag=f"yd{k}", name=f"yd{k}") for k in range(K4)]

        for ci, (c0, c1) in enumerate(TCH):
            ypsum = [scan_ps.tile([128, TC], FP32, tag=f"yps{j}", name=f"yps{j}_{ci}")
                     for j in range(NT)]
            for g in range(K4):
                Bb = [bbp.tile([128, NG * TC], BF16, tag=f"Bb{j}", name=f"Bb{j}_{ci}_{g}")
                      for j in range(NT)]
                Cb = [bbp.tile([128, NG * TC], BF16, tag=f"Cb{j}", name=f"Cb{j}_{ci}_{g}")
                      for j in range(NT)]
                for (j, o0, o1, k, d0, d1) in SECTIONS:
                    nc.sync.dma_start(
                        Bb[j][o0:o1, :],
                        B_dram[k * N + NG * g:k * N + NG * g + NG, c0:c1]
                        .partition_broadcast(o1 - o0))
                    nc.scalar.dma_start(
                        Cb[j][o0:o1, :],
                        C_dram[k * N + NG * g:k * N + NG * g + NG, c0:c1]
                        .partition_broadcast(o1 - o0))
                for n4 in range(NG):
                    n = NG * g + n4
                    for j in range(NT):
                        at = spool.tile([128, TC], BF16, tag=f"at{j}", name=f"at{j}")
                        nc.scalar.activation(at[:], dp[j][:, c0:c1], F.Exp,
                                             scale=cvc('Ap', j * N + n, j * N + n + 1))
                        bt = spool.tile([128, TC], BF16, tag=f"bt{j}", name=f"bt{j}")
                        nc.vector.tensor_mul(bt[:], xsp[j][:, c0:c1],
                                             Bb[j][:, n4 * TC:(n4 + 1) * TC])
                        ht = spool.tile([128, TC], BF16, tag=f"ht{j}", name=f"ht{j}")
                        if ci > 0:
                            # fold carry state into bt[0] so the scan can use
                            # the cheap zero-init form
                            nc.vector.scalar_tensor_tensor(
                                out=bt[:, 0:1], in0=at[:, 0:1],
                                scalar=state[j][:, n:n + 1], in1=bt[:, 0:1],
                                op0=A.mult, op1=A.add)
                        nc.vector.tensor_tensor_scan(ht[:], at[:], bt[:], 0.0,
                                                     A.mult, A.add)
                        if ci < 2:
                            nc.vector.tensor_copy(state[j][:, n:n + 1], ht[:, TC - 1:TC])
                        gt = spool.tile([128, TC], BF16, tag=f"gt{j}", name=f"gt{j}")
                        nc.vector.tensor_mul(gt[:], ht[:],
                                             Cb[j][:, n4 * TC:(n4 + 1) * TC])
                        for (s0, s1) in SUBS768:
                            nc.tensor.matmul(ypsum[j][:, s0:s1], cv('ident'),
                                             gt[:, s0:s1],
                                             start=(n == 0), stop=(n == N - 1))
            # drain this chunk's PSUM into per-direction scan-order tiles
            # (Act copies allow the partition shift)
            for (j, o0, o1, k, d0, d1) in SECTIONS:
                for (w0, w1) in split2(d0, o0, d1 - d0):
                    nc.scalar.copy(yd[k][d0 + w0:d0 + w1, c0:c1],
                                   ypsum[j][o0 + w0:o0 + w1, :])
        # merge directions into pixel order + D*u term
        tmp96 = stp.tile([HH, L], BF16, tag="tmp96")
        nc.vector.tensor_add(ysum[:], yd[0][:], yd[2][:, ::-1])
        nc.vector.tensor_add(tmp96[:], yd[1][:], yd[3][:, ::-1])
        nc.vector.tensor_add(ysum[:], ysum[:], whv(tmp96[:]))
        nc.vector.scalar_tensor_tensor(out=ysum[:], in0=xch[:], scalar=cv('Dsum'),
                                       in1=ysum[:], op0=A.mult, op1=A.add)
        nc.sync.dma_start(y_dram[:], ysum[:])
        nc.gpsimd.collective_compute(
            "AllGather", A.bypass, replica_groups=REPLICA_GROUPS,
            ins=[y_dram[:]], outs=[yg_dram[:]])
        sc.close()
        lngB.close()

        # ================= post =================
        po = ExitStack()
        post_ps = po.enter_context(tc.tile_pool(name="post_ps", bufs=4, space="PSUM"))
        pP = po.enter_context(tc.tile_pool(name="pP", bufs=1))
        rot = po.enter_context(tc.tile_pool(name="rot", bufs=4))

        # branch 2 + silu(z): no dep on the collective, runs under it
        g1 = rot.tile([48, L], BF16, tag="pb")
        for c0, c1 in MM_CHUNKS:
            ps = post_ps.tile([48, 512], FP32, tag="ps")
            nc.tensor.matmul(ps[:, :c1 - c0], cv('ag1T'), x2[:, c0:c1], start=True, stop=True)
            nc.scalar.activation(g1[:, c0:c1], ps[:, :c1 - c0], F.Relu, bias=cv('ag1b'))
        gat = rot.tile([CH, L], BF16, tag="pb")
        for c0, c1 in MM_CHUNKS:
            ps = post_ps.tile([CH, 512], FP32, tag="ps")
            nc.tensor.matmul(ps[:, :c1 - c0], cv('ag2T'), g1[:, c0:c1], start=True, stop=True)
            nc.scalar.activation(gat[:, c0:c1], ps[:, :c1 - c0], F.Sigmoid, bias=cv('ag2b'))
        x2g = pP.tile([CH, L], BF16, tag="x2g")
        nc.vector.tensor_mul(x2g[:], x2[:], gat[:])

        zsA = pP.tile([HH, L], BF16, tag="zsA")
        zsB = pP.tile([HH, L], BF16, tag="zsB")
        nc.scalar.activation(zsA[:], z0[0:96, :], F.Silu)
        nc.scalar.activation(zsB[0:32, :], z0[96:128, :], F.Silu)
        nc.scalar.activation(zsB[32:64, :], z1[0:32, :], F.Silu)
        nc.scalar.activation(zsB[64:96, :], z1[32:64, :], F.Silu)

        ygA = pP.tile([HH, L], BF16, tag="ygA")
        ygB = pP.tile([HH, L], BF16, tag="ygB")
        nc.sync.dma_start(ygA[:], yg_dram[0:HH, :])
        nc.sync.dma_start(ygB[:], yg_dram[HH:DI, :])

        sA = pP.tile([1, L], FP32, tag="sA")
        sB = pP.tile([1, L], FP32, tag="sB")
        sM = pP.tile([1, L], FP32, tag="sM")

        def ln_stats(cinv):
            # in: sA=raw sum, sB=raw sumsq; leaves rstd in sB (sA stays raw sum)
            nc.scalar.activation(sM[:], sA[:], F.Square, scale=cinv)
            nc.vector.scalar_tensor_tensor(out=sB[:], in0=sB[:], scalar=cinv,
                                           in1=sM[:], op0=A.mult, op1=A.subtract)
            nc.vector.tensor_scalar(out=sB[:], in0=sB[:], scalar1=1e-5,
                                    scalar2=None, op0=A.add)
            nc.vector.reciprocal(sB[:], sB[:])
            nc.scalar.activation(sB[:], sB[:], F.Sqrt)

        # LayerNorm over full DI (local stats via ones-matmul)
        ysqA = rot.tile([HH, L], BF16, tag="pb")
        ysqB = rot.tile([HH, L], BF16, tag="pb")
        nc.vector.tensor_mul(ysqA[:], ygA[:], ygA[:])
        nc.vector.tensor_mul(ysqB[:], ygB[:], ygB[:])
        for c, (c0, c1) in enumerate(MM_CHUNKS):
            ps = post_ps.tile([1, 512], FP32, tag="ps")
            nc.tensor.matmul(ps[:, :c1 - c0], ones96[:], ygA[:, c0:c1], start=True, stop=False)
            nc.tensor.matmul(ps[:, :c1 - c0], ones96[:], ygB[:, c0:c1], start=False, stop=True)
            nc.scalar.copy(sA[0:1, c0:c1], ps[:, :c1 - c0])
            ps2 = post_ps.tile([1, 512], FP32, tag="ps")
            nc.tensor.matmul(ps2[:, :c1 - c0], ones96[:], ysqA[:, c0:c1], start=True, stop=False)
            nc.tensor.matmul(ps2[:, :c1 - c0], ones96[:], ysqB[:, c0:c1], start=False, stop=True)
            nc.scalar.copy(sB[0:1, c0:c1], ps2[:, :c1 - c0])

        ln_stats(1.0 / DI)

        def apply_ln(pairs, bco):
            # pairs: list of (dst, src, gname, bname); bco: bcv column offset
            # holding 1/DI or 1/CH (folds the mean division into the
            # broadcast lhsT)
            for c0, c1 in MM_CHUNKS:
                cw = c1 - c0
                psm = post_ps.tile([HH, 512], FP32, tag="ps")
                nc.tensor.matmul(psm[:, :cw], cvc('bcv', bco, bco + HH),
                                 sA[:, c0:c1], start=True, stop=True)
                psr = post_ps.tile([HH, 512], FP32, tag="ps")
                nc.tensor.matmul(psr[:, :cw], cvc('bcv', 192, 192 + HH),
                                 sB[:, c0:c1], start=True, stop=True)
                for (dst, srct, gname, bname) in pairs:
                    nc.vector.tensor_sub(dst[:, c0:c1], srct[:, c0:c1],
                                         psm[:, :cw])
                    nc.vector.tensor_mul(dst[:, c0:c1], dst[:, c0:c1],
                                         psr[:, :cw])
                    nc.vector.tensor_scalar(out=dst[:, c0:c1], in0=dst[:, c0:c1],
                                            scalar1=cv(gname), scalar2=cv(bname),
                                            op0=A.mult, op1=A.add)

        ynA = rot.tile([HH, L], BF16, tag="pb")
        ynB = rot.tile([HH, L], BF16, tag="pb")
        apply_ln([(ynA, ygA, 'outngA', 'outnbA'),
                  (ynB, ygB, 'outngB', 'outnbB')], 0)

        gzA = rot.tile([HH, L], BF16, tag="pb")
        gzB = rot.tile([HH, L], BF16, tag="pb")
        nc.vector.tensor_mul(gzA[:], ynA[:], zsA[:])
        nc.vector.tensor_mul(gzB[:], ynB[:], zsB[:])

        x1o = pP.tile([CH, L], BF16, tag="x1o")
        for c0, c1 in MM_CHUNKS:
            ps = post_ps.tile([CH, 512], FP32, tag="ps")
            nc.tensor.matmul(ps[:, :c1 - c0], cv('outwTa'), gzA[:, c0:c1],
                             start=True, stop=False)
            nc.tensor.matmul(ps[:, :c1 - c0], cv('outwTb'), gzB[:, c0:c1],
                             start=False, stop=True)
            nc.scalar.copy(x1o[:, c0:c1], ps[:, :c1 - c0])

        yb = pP.tile([CH, L], BF16, tag="yb")
        nc.vector.tensor_add(yb[:], x1o[:], x2g[:])

        # local LayerNorm over channels
        ybsq = rot.tile([CH, L], BF16, tag="pb")
        nc.vector.tensor_mul(ybsq[:], yb[:], yb[:])
        for c, (c0, c1) in enumerate(MM_CHUNKS):
            ps = post_ps.tile([1, 512], FP32, tag="ps")
            nc.tensor.matmul(ps[:, :c1 - c0], ones96[:], yb[:, c0:c1], start=True, stop=True)
            nc.scalar.copy(sA[0:1, c0:c1], ps[:, :c1 - c0])
            ps2 = post_ps.tile([1, 512], FP32, tag="ps")
            nc.tensor.matmul(ps2[:, :c1 - c0], ones96[:], ybsq[:, c0:c1], start=True, stop=True)
            nc.scalar.copy(sB[0:1, c0:c1], ps2[:, :c1 - c0])
        ln_stats(1.0 / CH)
        ybn = pP.tile([CH, L], BF16, tag="ybn")
        apply_ln([(ybn, yb, 'lng', 'lnb')], 96)

        # CRM
        low_t = rot.tile([48, L], BF16, tag="pb")
        nc.sync.dma_start(low_t[:], ybn[48:96, :])
        upc = pP.tile([24, L], BF16, tag="upc")
        lowc = pP.tile([24, L], BF16, tag="lowc")
        for c0, c1 in MM_CHUNKS:
            ps = post_ps.tile([24, 512], FP32, tag="ps")
            nc.tensor.matmul(ps[:, :c1 - c0], cv('sq1T'), ybn[0:48, c0:c1], start=True, stop=True)
            nc.scalar.copy(upc[:, c0:c1], ps[:, :c1 - c0])
            ps2 = post_ps.tile([24, 512], FP32, tag="ps")
            nc.tensor.matmul(ps2[:, :c1 - c0], cv('sq2T'), low_t[:, c0:c1], start=True, stop=True)
            nc.scalar.copy(lowc[:, c0:c1], ps2[:, :c1 - c0])
        upcp = pP.tile([24, LP], BF16, tag="upcp")
        nc.gpsimd.memset(upcp[:], 0.0)
        nc.vector.tensor_copy(hwp(upcp[:])[:, 1:49, 1:49], hw(upc[:]))
        Y1 = pP.tile([CH, L], BF16, tag="Y1")
        m1c = pP.tile([CH, 5], FP32, tag="m1c")
        for ri, (r0, r1) in enumerate(ROW_CHUNKS):
            nr = r1 - r0
            ps = post_ps.tile([CH, 480], FP32, tag="ps")
            for tap in range(9):
                dy, dx = tap // 3, tap % 3
                rhs = hwp(upcp[:])[:, dy + r0:dy + r1, dx:dx + 48]
                nc.tensor.matmul(ps[:, :nr * 48], cvc('gwcT', tap * CH, (tap + 1) * CH),
                                 rhs, start=(tap == 0), stop=False)
            nc.tensor.matmul(ps[:, :nr * 48], cv('pw1T'), upc[:, r0 * 48:r1 * 48],
                             start=False, stop=True)
            nc.scalar.activation(Y1[:, r0 * 48:r1 * 48], ps[:, :nr * 48],
                                 F.Identity, bias=cv('gwcb'),
                                 accum_out=m1c[:, ri:ri + 1])
        Y2a = pP.tile([72, L], BF16, tag="Y2a")
        m2ca = pP.tile([72, 5], FP32, tag="m2ca")
        m2cb = pP.tile([24, 5], FP32, tag="m2cb")
        for ri, (c0, c1) in enumerate(MM_CHUNKS):
            ps = post_ps.tile([72, 512], FP32, tag="ps")
            nc.tensor.matmul(ps[:, :c1 - c0], cv('pw2T'), lowc[:, c0:c1], start=True, stop=True)
            nc.scalar.activation(Y2a[:, c0:c1], ps[:, :c1 - c0], F.Identity,
                                 accum_out=m2ca[:, ri:ri + 1])
            scr = post_ps.tile([24, 512], FP32, tag="ps")
            nc.scalar.activation(scr[:, :c1 - c0], lowc[:, c0:c1], F.Identity,
                                 accum_out=m2cb[:, ri:ri + 1])
        m1 = pP.tile([CH, 1], FP32, tag="m1")
        m2a_s = pP.tile([72, 1], FP32, tag="m2a_s")
        m2b_s = pP.tile([24, 1], FP32, tag="m2b_s")
        nc.vector.reduce_sum(m1[:], m1c[:], axis=mybir.AxisListType.X)
        nc.vector.reduce_sum(m2a_s[:], m2ca[:], axis=mybir.AxisListType.X)
        nc.vector.reduce_sum(m2b_s[:], m2cb[:], axis=mybir.AxisListType.X)
        smf = pP.tile([1, 2 * CH], FP32, tag="smf")
        nc.sync.dma_start(smf[0:1, 0:CH], m1[:, 0:1])
        nc.sync.dma_start(smf[0:1, CH:CH + 72], m2a_s[:, 0:1])
        nc.sync.dma_start(smf[0:1, CH + 72:2 * CH], m2b_s[:, 0:1])
        nc.vector.tensor_scalar(out=smf[:], in0=smf[:], scalar1=1.0 / L,
                                scalar2=None, op0=A.mult)
        mx = pP.tile([1, 1], FP32, tag="mx")
        nc.vector.reduce_max(mx[:], smf[:], axis=mybir.AxisListType.X)
        nc.vector.tensor_scalar(out=mx[:], in0=mx[:], scalar1=-1.0,
                                scalar2=None, op0=A.mult)
        nc.scalar.activation(smf[:], smf[:], F.Exp, bias=mx[0:1, 0:1])
        sm_s = pP.tile([1, 1], FP32, tag="sm_s")
        nc.vector.reduce_sum(sm_s[:], smf[:], axis=mybir.AxisListType.X)
        nc.vector.reciprocal(sm_s[:], sm_s[:])
        nc.vector.tensor_scalar(out=smf[:], in0=smf[:], scalar1=sm_s[0:1, 0:1],
                                scalar2=None, op0=A.mult)
        sm1 = pP.tile([CH, 1], FP32, tag="sm1")
        sm2 = pP.tile([CH, 1], FP32, tag="sm2")
        nc.sync.dma_start(sm1[:, 0:1], smf[0:1, 0:CH])
        nc.sync.dma_start(sm2[:, 0:1], smf[0:1, CH:2 * CH])
        o2f = rot.tile([CH, L], BF16, tag="pb")
        nc.sync.dma_start(o2f[0:72, :], Y2a[:])
        nc.sync.dma_start(o2f[72:96, :], lowc[:])
        o2t = pP.tile([CH, L], BF16, tag="o2t")
        nc.vector.tensor_scalar(out=o2t[:], in0=o2f[:], scalar1=sm2[:, 0:1],
                                scalar2=None, op0=A.mult)
        yc = pP.tile([CH, L], BF16, tag="yc")
        nc.vector.scalar_tensor_tensor(out=yc[:], in0=Y1[:], scalar=sm1[:, 0:1],
                                       in1=o2t[:], op0=A.mult, op1=A.add)
        outt = pP.tile([COUT, L], FP32, tag="outt")
        for c0, c1 in MM_CHUNKS:
            ps = post_ps.tile([COUT, 512], FP32, tag="ps")
            nc.tensor.matmul(ps[:, :c1 - c0], cv('finT'), yc[:, c0:c1], start=True, stop=True)
            nc.scalar.activation(outt[:, c0:c1], ps[:, :c1 - c0], F.Identity, bias=cv('finb'))
        nc.sync.dma_start(out_d[:], outt[:])
        po.close()
        glob.close()
    split_multi_waits(nc, max_waits=1)
    return nc


# =============================== host side ==================================

def prep_core_inputs(inputs, b, half):
    import ml_dtypes
    f32 = np.float32
    bf16 = ml_dtypes.bfloat16
    d0 = half * HH

    def asf(a):
        return np.asarray(a, f32)

    bnscale = asf(inputs['bn_g']) / np.sqrt(np.float32(1.0 + 1e-5))
    w1 = asf(inputs['conv1_w'])[:, :, 0, 0] * bnscale[:, None]
    b1 = asf(inputs['conv1_b']) * bnscale + asf(inputs['bn_b'])

    def diag9(w, nch):
        out = np.zeros((nch, 9 * nch), f32)
        w = asf(w)
        for tap in range(9):
            dy, dx = tap // 3, tap % 3
            blk = out[:, tap * nch:(tap + 1) * nch]
            np.fill_diagonal(blk, w[:, 0, dy, dx])
        return out

    sscd = diag9(inputs['ss_conv_w'], DI)        # (192, 9*192)
    sc0 = np.zeros((128, 9 * 128), f32)
    sc1 = np.zeros((64, 9 * 64), f32)
    for tap in range(9):
        blk = sscd[:, tap * DI:(tap + 1) * DI]
        sc0[:, tap * 128:(tap + 1) * 128] = blk[0:128, 0:128]
        sc1[:, tap * 64:(tap + 1) * 64] = blk[128:192, 128:192]

    sel = np.zeros((DI, HH), f32)
    sel[np.arange(d0, d0 + HH), np.arange(HH)] = 1.0

    xp = asf(inputs['ss_xproj_w'])               # (4, 38, 192)
    xpTa = np.zeros((128, K4 * 38), f32)
    xpTb = np.zeros((64, K4 * 38), f32)
    for k in range(K4):
        xpT = xp[k].T                            # (192, 38)
        xpTa[:, k * 38:(k + 1) * 38] = xpT[0:128]
        xpTb[:, k * 38:(k + 1) * 38] = xpT[128:192]

    dtw = asf(inputs['ss_dt_w'])
    dtwT = np.zeros((R, K4 * HH), f32)
    for k in range(K4):
        dtwT[:, k * HH:(k + 1) * HH] = dtw[k][d0:d0 + HH, :].T

    dtb_full = asf(inputs['ss_dt_b'])
    Alog = asf(inputs['ss_Alog']).reshape(K4, DI, N)
    Dv = asf(inputs['ss_D']).reshape(K4, DI)
    dtb_p = np.zeros((128, NT), f32)
    Ap = np.zeros((128, NT * N), f32)
    for (j, o0, o1, k, dd0, dd1) in SECTIONS:
        dtb_p[o0:o1, j] = dtb_full[k, d0 + dd0:d0 + dd1]
        Ap[o0:o1, j * N:(j + 1) * N] = -np.exp(Alog[k, d0 + dd0:d0 + dd1])
    Dsum = Dv[:, d0:d0 + HH].sum(0)[:, None]

    gw = asf(inputs['gwc_w'])
    gT = np.zeros((24, 9 * CH), f32)
    for tap in range(9):
        dy, dx = tap // 3, tap % 3
        blk = np.zeros((24, CH), f32)
        blk[0:12, 0:48] = gw[0:48, :, dy, dx].T
        blk[12:24, 48:96] = gw[48:96, :, dy, dx].T
        gT[:, tap * CH:(tap + 1) * CH] = blk

    owT = asf(inputs['ss_out_w']).T              # (192, 96)
    outn_g = asf(inputs['ss_outn_g'])
    outn_b = asf(inputs['ss_outn_b'])

    vals32 = {
        'w1T': w1.T, 'b1': b1[:, None],
        'linb': asf(inputs['lin_b'])[:, None],
        'dw1b': asf(inputs['dw1_b'])[:, None],
        'dw2b': asf(inputs['dw2_b'])[:, None],
        'scb0': asf(inputs['ss_conv_b'])[0:128, None],
        'scb1': asf(inputs['ss_conv_b'])[128:192, None],
        'dtb': dtb_p, 'Ap': Ap, 'Dsum': Dsum,
        'outngA': outn_g[0:96, None], 'outngB': outn_g[96:192, None],
        'outnbA': outn_b[0:96, None], 'outnbB': outn_b[96:192, None],
        'ag1b': asf(inputs['ag1_b'])[:, None],
        'ag2b': asf(inputs['ag2_b'])[:, None],
        'lng': asf(inputs['ln_g'])[:, None],
        'lnb': asf(inputs['ln_b'])[:, None],
        'gwcb': asf(inputs['gwc_b'])[:, None],
        'finb': asf(inputs['fin_b'])[:, None],
        'bcv': np.concatenate([np.full((1, 96), 1.0 / DI, f32),
                               np.full((1, 96), 1.0 / CH, f32),
                               np.ones((1, 96), f32)], axis=1),
    }
    valsbf = {
        'linT': asf(inputs['lin_w']).T,
        'dw1dg': diag9(inputs['dw1_w'], CH),
        'dw2dg': diag9(inputs['dw2_w'], CH),
        'inwT': asf(inputs['ss_in_w']).T,        # (96, 384) full z
        'sc0dg': sc0, 'sc1dg': sc1,
        'sel0': sel[0:128], 'sel1': sel[128:192],
        'xpTa': xpTa, 'xpTb': xpTb,
        'dtwT': dtwT,
        'ident': np.eye(128, dtype=f32),
        'outwTa': owT[0:96], 'outwTb': owT[96:192],
        'ag1T': asf(inputs['ag1_w'])[:, :, 0, 0].T,
        'ag2T': asf(inputs['ag2_w'])[:, :, 0, 0].T,
        'sq1T': asf(inputs['sq1_w'])[:, :, 0, 0].T,
        'sq2T': asf(inputs['sq2_w'])[:, :, 0, 0].T,
        'gwcT': gT,
        'pw1T': asf(inputs['pwc1_w'])[:, :, 0, 0].T,
        'pw2T': asf(inputs['pwc2_w'])[:, :, 0, 0].T,
        'finT': asf(inputs['fin_w']).T,
    }

    blob32 = np.zeros((128, W32), f32)
    for nm, p, c in CONSTS_F32:
        o = OFF32[nm][0]
        v = vals32[nm]
        assert v.shape == (p, c), (nm, v.shape, (p, c))
        blob32[0:p, o:o + c] = v
    blobbf = np.zeros((128, WBF), bf16)
    for nm, p, c in CONSTS_BF16:
        o = OFFBF[nm][0]
        v = valsbf[nm]
        assert v.shape == (p, c), (nm, v.shape, (p, c))
        blobbf[0:p, o:o + c] = v.astype(bf16)

    return {
        'x': np.ascontiguousarray(asf(inputs['x'])[b].reshape(CIN, L)),
        'c32': blob32,
        'cbf': np.ascontiguousarray(blobbf),
    }


_NC_CACHE = {}


def get_nc():
    if 'nc' not in _NC_CACHE:
        _NC_CACHE['nc'] = build_nc()
    return _NC_CACHE['nc']


def kernel(**inputs):
    from concourse.bass_utils import run_bass_kernel_spmd
    nc = get_nc()
    in_maps = [prep_core_inputs(inputs, c // 2, c % 2) for c in range(8)]
    res = run_bass_kernel_spmd(nc, in_maps, core_ids=list(range(8)))
    out = np.zeros((B_, COUT, H, W), np.float32)
    for b in range(B_):
        out[b] = res.results[2 * b]['out'].reshape(COUT, H, W)
    return out


# revision 55
# speedup vs baseline: 1.0136x; 1.0084x over previous
"""Self-contained Trainium2 Bass kernel for the CR-VSS block (8 cores)."""

# ---- TileContext drain-wait patch (walrus 1-wait limit) ----
"""Patch TileContext._drain_and_barrier: the axon-client walrus rejects
instructions carrying >2 sem waits ("Too many sync wait commands" in
setupSyncWait for CTRL structs). Redistribute the exit-drain's waits across
preceding SP nop instructions, each carrying at most MAX_WAITS."""
from concourse.tile import TileContext, ScopedClock

MAX_WAITS = 1


def _patched_drain_and_barrier(self, tick_clock, wait_clock):
    nc = self.nc
    drain_inst = nc.sync.drain()
    wait_clock.add_sem_waits(
        drain_inst.ins, ScopedClock({None: tick_clock.global_clock})
    )

    waits = list(drain_inst.ins.sync_info.on_wait or [])
    if len(waits) > MAX_WAITS:
        bb = nc.cur_bb.bb
        assert bb.instructions[-1] is drain_inst.ins
        # strip waits from the drain, re-emit them on nop carriers
        drain_inst.ins.sync_info.on_wait = waits[:0]
        carriers = []
        import concourse.mybir as mybir
        for i in range(0, len(waits), MAX_WAITS):
            nop = nc.sync.nop(nofuse=True)
            nop.ins.sync_info = mybir.SyncInfo(
                on_wait=waits[i:i + MAX_WAITS], on_update=[]
            )
            carriers.append(nop.ins)
        # move carriers before the drain
        insts = list(bb.instructions)
        assert insts[-len(carriers) - 1] is drain_inst.ins
        reordered = insts[:-len(carriers) - 1] + insts[-len(carriers):] + [drain_inst.ins]
        while len(bb.instructions):
            bb.instructions.pop()
        for x in reordered:
            bb.instructions.append(x)

    nc.all_engine_barrier()
    assert self.sems is not None
    popped = nc._tile_sem_poison_stack.pop()
    assert popped is self._sem_poison
    nc.clear_and_free_semaphores(list(self.sems.allocated().values()))
    nc.all_engine_barrier()


def apply():
    TileContext._drain_and_barrier = _patched_drain_and_barrier


def split_multi_waits(nc, max_waits=1):
    """Post-pass: walrus CTRL codegen rejects instructions with more than
    one sem wait. Move extra waits onto same-engine NoOp carriers."""
    import concourse.mybir as mybir
    for f in nc.m.functions:
        for bb in f.blocks:
            insts = list(bb.instructions)
            out = []
            changed = False
            for ins in insts:
                si = ins.sync_info
                if si is not None and si.on_wait and len(si.on_wait) > max_waits:
                    waits = list(si.on_wait)
                    for i, w in enumerate(waits[max_waits:]):
                        nop = mybir.InstNoOp.__new__(
                            mybir.InstNoOp, name=f"{ins.name}-xw{i}", ins=[], outs=[])
                        nop.engine = ins.engine
                        nop.sync_info = mybir.SyncInfo(on_wait=[w], on_update=[])
                        out.append(nop)
                    ins.sync_info = mybir.SyncInfo(
                        on_wait=waits[:max_waits],
                        on_update=list(si.on_update or []))
                    changed = True
                out.append(ins)
            if changed:
                while len(bb.instructions):
                    bb.instructions.pop()
                for x in out:
                    bb.instructions.append(x)

apply()

# ---- kernel ----
"""Trainium2 Bass kernel for nn_CR_VSS (VSS block with SS2D selective scan).

Sharding: 8 cores = 4 samples x 2 d_inner-halves. Each core runs the full
pre-stage for its sample, scans its 96-channel d-half across all 4
cross-scan directions (packed into 3x128-partition tiles), then the pair
exchanges y-halves with ONE AllGather; LN + out-proj + post-stage run
locally (z is computed full-width in the in-proj so no second collective).

Scan: h_t = exp(A*delta_t)*h_{t-1} + delta_t*u_t*B_t per (k,d,n) via
tensor_tensor_scan; n in groups of 4 with batched B/C partition-broadcast
DMAs (double-buffered); y accumulated over n with identity-lhsT PSUM
matmuls, merged into pixel-order ysum straight from PSUM per t-chunk.
"""
import numpy as np
from contextlib import ExitStack

import concourse.bass as bass
import concourse.mybir as mybir

F = mybir.ActivationFunctionType
A = mybir.AluOpType
FP32 = mybir.dt.float32
BF16 = mybir.dt.bfloat16

B_, CIN, CH, COUT, H, W = 4, 96, 96, 96, 48, 48
DI, N, R, K4 = 192, 16, 6, 4
L = H * W               # 2304
HH = 96                 # d-half per core
NT = 3                  # packed (k,d) tiles: 4*96 = 384 = 3*128
HP = 50
LP = 2500
TC = 768                # scan t-chunk (16 rows of 48)
TCH = [(0, 768), (768, 1536), (1536, 2304)]
NG = 4                  # scan n-group (broadcast batch)

# packed (k,d) rows -> (tile j, offset): sections (j, o0, o1, k, d0, d1).
# Section offsets are all 0/32/64 so PE matmuls can write them directly.
SECTIONS = [
    (0, 0, 32, 1, 0, 32),
    (0, 32, 128, 0, 0, 96),
    (1, 0, 64, 1, 32, 96),
    (1, 64, 128, 2, 0, 64),
    (2, 0, 32, 2, 64, 96),
    (2, 32, 128, 3, 0, 96),
]

MM_CHUNKS = [(0, 512), (512, 1024), (1024, 1536), (1536, 2048), (2048, 2304)]
ROW_CHUNKS = [(0, 10), (10, 20), (20, 30), (30, 40), (40, 48)]
SUBS768 = [(0, 512), (512, 768)]
INW_BLOCKS = [(0, 128), (128, 256), (256, 384)]

REPLICA_GROUPS = [[0, 1], [2, 3], [4, 5], [6, 7]]

# ---- const blobs (shared layout between host packing and kernel views) ----
CONSTS_F32 = [
    ('w1T', 96, 96), ('b1', 96, 1), ('linb', 96, 1),
    ('dw1b', 96, 1), ('dw2b', 96, 1),
    ('scb0', 128, 1), ('scb1', 64, 1),
    ('dtb', 128, 3), ('Ap', 128, 48), ('Dsum', 96, 1),
    ('outngA', 96, 1), ('outngB', 96, 1), ('outnbA', 96, 1), ('outnbB', 96, 1),
    ('ag1b', 48, 1), ('ag2b', 96, 1), ('lng', 96, 1), ('lnb', 96, 1),
    ('gwcb', 96, 1), ('finb', 96, 1), ('bcv', 1, 288),
]
CONSTS_BF16 = [
    ('linT', 96, 96),
    ('dw1dg', 96, 864), ('dw2dg', 96, 864),
    ('inwT', 96, 384),
    ('sc0dg', 128, 1152), ('sc1dg', 64, 576),
    ('sel0', 128, 96), ('sel1', 64, 96),
    ('xpTa', 128, 152), ('xpTb', 64, 152),
    ('dtwT', 6, 384),
    ('ident', 128, 128),
    ('outwTa', 96, 96), ('outwTb', 96, 96),
    ('ag1T', 96, 48), ('ag2T', 48, 96),
    ('sq1T', 48, 24), ('sq2T', 48, 24),
    ('gwcT', 24, 864), ('pw1T', 24, 96), ('pw2T', 24, 72),
    ('finT', 96, 96),
]

OFF32 = {}
_o = 0
for _nm, _p, _c in CONSTS_F32:
    OFF32[_nm] = (_o, _p, _c)
    _o += _c
W32 = _o
OFFBF = {}
_o = 0
for _nm, _p, _c in CONSTS_BF16:
    OFFBF[_nm] = (_o, _p, _c)
    _o += _c
WBF = _o


def build_nc():
    nc = bass.Bass(trn_type="TRN2", num_devices=8)

    x_d = nc.dram_tensor("x", [CIN, L], FP32, kind="ExternalInput")
    c32_d = nc.dram_tensor("c32", [128, W32], FP32, kind="ExternalInput")
    cbf_d = nc.dram_tensor("cbf", [128, WBF], BF16, kind="ExternalInput")
    out_d = nc.dram_tensor("out", [COUT, L], FP32, kind="ExternalOutput")

    B_dram = nc.dram_tensor("B_dram", [K4 * N, L], BF16)
    C_dram = nc.dram_tensor("C_dram", [K4 * N, L], BF16)
    y_dram = nc.dram_tensor("y_dram", [HH, L], BF16)
    yg_dram = nc.dram_tensor("yg_dram", [DI, L], BF16)
    st_dram = nc.dram_tensor("st_dram", [2, L], BF16)

    def hw(ap):
        return ap.rearrange("p (h w) -> p h w", h=H)

    def hwp(ap):
        return ap.rearrange("p (h w) -> p h w", h=HP)

    def whv(ap):
        return ap.rearrange("p (h w) -> p w h", h=H)

    with TileContext(nc) as tc:
        glob = ExitStack()
        cst = glob.enter_context(tc.tile_pool(name="cst", bufs=1))
        lngA = glob.enter_context(tc.tile_pool(name="lngA", bufs=1))

        cst32 = cst.tile([128, W32], FP32, tag="cst32")
        cstbf = cst.tile([128, WBF], BF16, tag="cstbf")
        nc.sync.dma_start(cst32[:], c32_d[:])
        nc.sync.dma_start(cstbf[:], cbf_d[:])

        def cvc(nm, a0=0, a1=None, p0=0, p1=None):
            d, tile = (OFF32, cst32) if nm in OFF32 else (OFFBF, cstbf)
            o, p, c = d[nm]
            if a1 is None:
                a1 = c
            if p1 is None:
                p1 = p
            return tile[p0:p1, o + a0:o + a1]

        cv = cvc

        ones96 = cst.tile([HH, 1], BF16, tag="ones96")
        nc.vector.memset(ones96[:], 1.0)

        # long-lived across phases
        z0 = lngA.tile([128, L], BF16, tag="z0")     # z rows 0:128
        z1 = lngA.tile([64, L], BF16, tag="z1")      # z rows 128:192
        x2 = lngA.tile([CH, L], BF16, tag="x2")
        lngB = ExitStack()
        lngB_p = lngB.enter_context(tc.tile_pool(name="lngB_p", bufs=1))
        xch = lngB_p.tile([HH, L], BF16, tag="xch")
        dp = [lngB_p.tile([128, L], BF16, tag=f"dp{j}", name=f"dp{j}") for j in range(NT)]
        # xsp holds packed scan-order xs, overwritten in place with delta*u
        xsp = [lngB_p.tile([128, L], BF16, tag=f"xsp{j}", name=f"xsp{j}") for j in range(NT)]
        ysum = lngB_p.tile([HH, L], BF16, tag="ysum")

        # ================= pre-stage =================
        pre = ExitStack()
        pre_ps = pre.enter_context(tc.tile_pool(name="pre_ps", bufs=4, space="PSUM"))
        pA = pre.enter_context(tc.tile_pool(name="pA", bufs=1))
        pB = pre.enter_context(tc.tile_pool(name="pB", bufs=1))

        xt = pA.tile([CIN, L], FP32, tag="xt")
        nc.sync.dma_start(xt[:], x_d[:])

        # conv1x1 (+folded BN) + ReLU
        h1 = pA.tile([CH, L], BF16, tag="h1")
        for c0, c1 in MM_CHUNKS:
            ps = pre_ps.tile([CH, 512], FP32, tag="ps")
            nc.tensor.matmul(ps[:, :c1 - c0], cv('w1T'), xt[:, c0:c1], start=True, stop=True)
            nc.scalar.activation(h1[:, c0:c1], ps[:, :c1 - c0], F.Relu, bias=cv('b1'))
        # token linear
        h2 = pA.tile([CH, L], BF16, tag="h2")
        for c0, c1 in MM_CHUNKS:
            ps = pre_ps.tile([CH, 512], FP32, tag="ps")
            nc.tensor.matmul(ps[:, :c1 - c0], cv('linT'), h1[:, c0:c1], start=True, stop=True)
            nc.vector.tensor_scalar(out=h2[:, c0:c1], in0=ps[:, :c1 - c0],
                                    scalar1=cv('linb'), scalar2=None, op0=A.add)
        h2p = pA.tile([CH, LP], BF16, tag="h2p")
        nc.gpsimd.memset(h2p[:], 0.0)
        for (r0, r1) in ROW_CHUNKS:
            nc.vector.tensor_copy(hwp(h2p[:])[:, r0 + 1:r1 + 1, 1:49],
                                  hw(h2[:])[:, r0:r1, :])

        def dwconv(dst, src_p, dgname, biasname, nch):
            for (r0, r1) in ROW_CHUNKS:
                nr = r1 - r0
                ps = pre_ps.tile([128, 480], FP32, tag="ps")
                for tap in range(9):
                    dy, dx = tap // 3, tap % 3
                    rhs = hwp(src_p[:])[:, dy + r0:dy + r1, dx:dx + 48]
                    nc.tensor.matmul(ps[:nch, :nr * 48],
                                     cvc(dgname, tap * nch, (tap + 1) * nch),
                                     rhs, start=(tap == 0), stop=(tap == 8))
                nc.scalar.activation(dst[:, r0 * 48:r1 * 48], ps[:nch, :nr * 48],
                                     F.Silu, bias=cv(biasname))

        x1 = pB.tile([CH, L], BF16, tag="x1")
        dwconv(x1, h2p, 'dw1dg', 'dw1b', CH)

        # in-proj: xi (192) + FULL z (192)
        xi0 = pB.tile([128, L], BF16, tag="xi0")
        xi1 = pB.tile([64, L], BF16, tag="xi1")
        for mb, (m0, m1) in enumerate(INW_BLOCKS):
            for c0, c1 in MM_CHUNKS:
                ps = pre_ps.tile([128, 512], FP32, tag="ps")
                nc.tensor.matmul(ps[:m1 - m0, :c1 - c0], cvc('inwT', m0, m1),
                                 x1[:, c0:c1], start=True, stop=True)
                if mb == 0:
                    nc.vector.tensor_copy(xi0[:, c0:c1], ps[:128, :c1 - c0])
                elif mb == 1:
                    nc.scalar.copy(xi1[:, c0:c1], ps[0:64, :c1 - c0])
                    nc.scalar.copy(z0[0:64, c0:c1], ps[64:128, :c1 - c0])
                else:
                    nc.scalar.copy(z0[64:128, c0:c1], ps[0:64, :c1 - c0])
                    nc.scalar.copy(z1[0:64, c0:c1], ps[64:128, :c1 - c0])

        xi0p = pB.tile([128, LP], BF16, tag="xi0p")
        xi1p = pB.tile([64, LP], BF16, tag="xi1p")
        nc.gpsimd.memset(xi0p[:], 0.0)
        nc.gpsimd.memset(xi1p[:], 0.0)
        for (r0, r1) in ROW_CHUNKS:
            nc.vector.tensor_copy(hwp(xi0p[:])[:, r0 + 1:r1 + 1, 1:49],
                                  hw(xi0[:])[:, r0:r1, :])
            nc.vector.tensor_copy(hwp(xi1p[:])[:, r0 + 1:r1 + 1, 1:49],
                                  hw(xi1[:])[:, r0:r1, :])
        xc0 = pB.tile([128, L], BF16, tag="xc0")
        xc1 = pB.tile([64, L], BF16, tag="xc1")
        dwconv(xc0, xi0p, 'sc0dg', 'scb0', 128)
        dwconv(xc1, xi1p, 'sc1dg', 'scb1', 64)

        # d-half extraction + wh copy
        for c0, c1 in MM_CHUNKS:
            ps = pre_ps.tile([HH, 512], FP32, tag="ps")
            nc.tensor.matmul(ps[:, :c1 - c0], cv('sel0'), xc0[:, c0:c1], start=True, stop=False)
            nc.tensor.matmul(ps[:, :c1 - c0], cv('sel1'), xc1[:, c0:c1], start=False, stop=True)
            nc.vector.tensor_copy(xch[:, c0:c1], ps[:, :c1 - c0])
        xwhh = pB.tile([HH, L], BF16, tag="xwhh")
        for (t0, t1) in TCH:
            w0, w1 = t0 // 48, t1 // 48
            nc.vector.tensor_copy(hw(xwhh[:])[:, w0:w1, :],
                                  whv(xch[:])[:, w0:w1, :])

        # xproj (compact 38 rows: 0:6 dts, 6:22 B, 22:38 C) in scan order
        def xc_read(k, c0, c1):
            if k == 0:
                return (xc0[:, c0:c1], xc1[:, c0:c1])
            if k == 1:
                return (whv(xc0[:])[:, c0 // 48:c1 // 48, :],
                        whv(xc1[:])[:, c0 // 48:c1 // 48, :])
            if k == 2:
                return (xc0[:, L - c1:L - c0][:, ::-1],
                        xc1[:, L - c1:L - c0][:, ::-1])
            r0 = whv(xc0[:])[:, (L - c1) // 48:(L - c0) // 48, :][:, ::-1, ::-1]
            r1 = whv(xc1[:])[:, (L - c1) // 48:(L - c0) // 48, :][:, ::-1, ::-1]
            return (r0, r1)

        # row-chunk outer so all 4 directions' early columns finish first;
        # B/C are written to DRAM per scan chunk so ci=0 broadcasts can
        # start while xproj still works on later chunks.
        stage = [pB.tile([38, L], BF16, tag=f"stg{k}", name=f"stg{k}") for k in range(K4)]
        done_w = 0
        for ri, (rr0, rr1) in enumerate(ROW_CHUNKS):
            c0, c1 = rr0 * 48, rr1 * 48
            nf = c1 - c0
            for k in range(K4):
                ra, rb = xc_read(k, c0, c1)
                ps = pre_ps.tile([38, 480], FP32, tag="ps")
                nc.tensor.matmul(ps[:, :nf], cvc('xpTa', k * 38, (k + 1) * 38), ra,
                                 start=True, stop=False)
                nc.tensor.matmul(ps[:, :nf], cvc('xpTb', k * 38, (k + 1) * 38), rb,
                                 start=False, stop=True)
                nc.vector.tensor_copy(stage[k][:, c0:c1], ps[:, :nf])
            while done_w < len(TCH) and TCH[done_w][1] <= c1:
                t0, t1 = TCH[done_w]
                for k in range(K4):
                    nc.sync.dma_start(B_dram[k * N:(k + 1) * N, t0:t1],
                                      stage[k][6:22, t0:t1])
                    nc.sync.dma_start(C_dram[k * N:(k + 1) * N, t0:t1],
                                      stage[k][22:38, t0:t1])
                done_w += 1

        # delta: packed matmuls then softplus on full 128-partition tiles
        def mm_windows(a0, a1):
            if a0 == 0:
                return [(0, a1)]
            res = []
            x = a0
            while x < a1:
                if x % 64 == 32:
                    e = min(a1, x + 32)
                else:  # x == 64
                    e = min(a1, 128)
                res.append((x, e))
                x = e
            return res

        for (cc0, cc1) in MM_CHUNKS:
            cw = cc1 - cc0
            for j in range(NT):
                ex = pre_ps.tile([128, 512], FP32, tag="ps")
                for (jj, o0, o1, k, d0, d1) in SECTIONS:
                    if jj != j:
                        continue
                    for (w0, w1) in mm_windows(o0, o1):
                        dd0 = d0 + (w0 - o0)
                        dd1 = d0 + (w1 - o0)
                        nc.tensor.matmul(ex[w0:w1, :cw],
                                         cvc('dtwT', k * 96 + dd0, k * 96 + dd1),
                                         stage[k][0:6, cc0:cc1], start=True, stop=True)
                # softplus(x+b) = ln(1 + exp(x+b)) (no softplus act table on HW)
                ex2 = pre_ps.tile([128, 512], FP32, tag="ps")
                nc.scalar.activation(ex2[:, :cw], ex[:, :cw], F.Exp,
                                     bias=cvc('dtb', j, j + 1))
                nc.scalar.activation(dp[j][:, cc0:cc1], ex2[:, :cw], F.Ln, bias=1.0)

        # pack scan-order xs (Act copies handle partition shift + flips),
        # then overwrite in place with delta*u = dp*xs.
        # Act partition windows must not cross engine block boundaries on
        # EITHER side: allowed starts 0/32/64/96; a start-32 window may not
        # cross 64. split2 chops a shifted copy accordingly.
        def _legal_span(s):
            return 32 if s == 32 else 128 - s if s else 128

        def split2(o0, i0, ln):
            res = []
            x = 0
            while x < ln:
                step = min(ln - x, _legal_span(o0 + x), _legal_span(i0 + x))
                res.append((x, x + step))
                x += step
            return res

        for (t0, t1) in TCH:
            for (j, o0, o1, k, d0, d1) in SECTIONS:
                v = xwhh if k in (1, 3) else xch
                if k < 2:
                    # forward sections: contiguous rows, cheap DMA shift
                    nc.sync.dma_start(xsp[j][o0:o1, t0:t1], v[d0:d1, t0:t1])
                    continue
                for (w0, w1) in split2(o0, d0, o1 - o0):
                    nc.scalar.copy(xsp[j][o0 + w0:o0 + w1, t0:t1],
                                   v[d0 + w0:d0 + w1, ::-1][:, t0:t1])
            for j in range(NT):
                nc.vector.tensor_mul(xsp[j][:, t0:t1], dp[j][:, t0:t1],
                                     xsp[j][:, t0:t1])

        # branch-2 dwconv last: x2 is only needed in the post stage, so
        # this fills PE/Act slack once the scan inputs are queued
        dwconv(x2, h2p, 'dw2dg', 'dw2b', CH)

        pre.close()

        # ================= scan =================
        sc = ExitStack()
        bbp = sc.enter_context(tc.tile_pool(name="bbp", bufs=2))
        spool = sc.enter_context(tc.tile_pool(name="spool", bufs=2))
        scan_ps = sc.enter_context(tc.tile_pool(name="scan_ps", bufs=1, space="PSUM"))
        stp = sc.enter_context(tc.tile_pool(name="stp", bufs=1))
        state = [stp.tile([128, N], FP32, tag=f"st{j}", name=f"state{j}")
                 for j in range(NT)]
        yd = [stp.tile([HH, L], BF16, tag=f"yd{k}", name=f"yd{k}") for k in range(K4)]

        pending_drain = None
        for ci, (c0, c1) in enumerate(TCH):
            ypsum = [scan_ps.tile([128, TC], FP32, tag=f"yps{j}", name=f"yps{j}_{ci}")
                     for j in range(NT)]
            for g in range(K4):
                Bb = [bbp.tile([128, NG * TC], BF16, tag=f"Bb{j}", name=f"Bb{j}_{ci}_{g}")
                      for j in range(NT)]
                Cb = [bbp.tile([128, NG * TC], BF16, tag=f"Cb{j}", name=f"Cb{j}_{ci}_{g}")
                      for j in range(NT)]
                for (j, o0, o1, k, d0, d1) in SECTIONS:
                    nc.sync.dma_start(
                        Bb[j][o0:o1, :],
                        B_dram[k * N + NG * g:k * N + NG * g + NG, c0:c1]
                        .partition_broadcast(o1 - o0))
                    nc.scalar.dma_start(
                        Cb[j][o0:o1, :],
                        C_dram[k * N + NG * g:k * N + NG * g + NG, c0:c1]
                        .partition_broadcast(o1 - o0))
                if pending_drain is not None:
                    pending_drain()
                    pending_drain = None
                for n4 in range(NG):
                    n = NG * g + n4
                    for j in range(NT):
                        at = spool.tile([128, TC], BF16, tag=f"at{j}", name=f"at{j}")
                        nc.scalar.activation(at[:], dp[j][:, c0:c1], F.Exp,
                                             scale=cvc('Ap', j * N + n, j * N + n + 1))
                        bt = spool.tile([128, TC], BF16, tag=f"bt{j}", name=f"bt{j}")
                        nc.vector.tensor_mul(bt[:], xsp[j][:, c0:c1],
                                             Bb[j][:, n4 * TC:(n4 + 1) * TC])
                        ht = spool.tile([128, TC], BF16, tag=f"ht{j}", name=f"ht{j}")
                        if ci > 0:
                            # fold carry state into bt[0] so the scan can use
                            # the cheap zero-init form
                            nc.vector.scalar_tensor_tensor(
                                out=bt[:, 0:1], in0=at[:, 0:1],
                                scalar=state[j][:, n:n + 1], in1=bt[:, 0:1],
                                op0=A.mult, op1=A.add)
                        nc.vector.tensor_tensor_scan(ht[:], at[:], bt[:], 0.0,
                                                     A.mult, A.add)
                        if ci < 2:
                            nc.vector.tensor_copy(state[j][:, n:n + 1], ht[:, TC - 1:TC])
                        gt = spool.tile([128, TC], BF16, tag=f"gt{j}", name=f"gt{j}")
                        nc.vector.tensor_mul(gt[:], ht[:],
                                             Cb[j][:, n4 * TC:(n4 + 1) * TC])
                        for (s0, s1) in SUBS768:
                            nc.tensor.matmul(ypsum[j][:, s0:s1], cv('ident'),
                                             gt[:, s0:s1],
                                             start=(n == 0), stop=(n == N - 1))
            # drain this chunk's PSUM into per-direction scan-order tiles
            # (Act copies allow the partition shift). Deferred past the next
            # chunk's broadcast issue so the boundary doesn't stall Act.
            def _drain(yps=ypsum, cc0=c0, cc1=c1):
                for (j, o0, o1, k, d0, d1) in SECTIONS:
                    for (w0, w1) in split2(d0, o0, d1 - d0):
                        nc.scalar.copy(yd[k][d0 + w0:d0 + w1, cc0:cc1],
                                       yps[j][o0 + w0:o0 + w1, :])
            pending_drain = _drain
        if pending_drain is not None:
            pending_drain()
            pending_drain = None
        # merge directions into pixel order + D*u term
        tmp96 = stp.tile([HH, L], BF16, tag="tmp96")
        nc.vector.tensor_add(ysum[:], yd[0][:], yd[2][:, ::-1])
        nc.vector.tensor_add(tmp96[:], yd[1][:], yd[3][:, ::-1])
        nc.vector.tensor_add(ysum[:], ysum[:], whv(tmp96[:]))
        nc.vector.scalar_tensor_tensor(out=ysum[:], in0=xch[:], scalar=cv('Dsum'),
                                       in1=ysum[:], op0=A.mult, op1=A.add)
        nc.sync.dma_start(y_dram[:], ysum[:])
        nc.gpsimd.collective_compute(
            "AllGather", A.bypass, replica_groups=REPLICA_GROUPS,
            ins=[y_dram[:]], outs=[yg_dram[:]])
        sc.close()
        lngB.close()

        # ================= post =================
        po = ExitStack()
        post_ps = po.enter_context(tc.tile_pool(name="post_ps", bufs=4, space="PSUM"))
        pP = po.enter_context(tc.tile_pool(name="pP", bufs=1))
        rot = po.enter_context(tc.tile_pool(name="rot", bufs=4))

        # branch 2 + silu(z): no dep on the collective, runs under it
        g1 = rot.tile([48, L], BF16, tag="pb")
        for c0, c1 in MM_CHUNKS:
            ps = post_ps.tile([48, 512], FP32, tag="ps")
            nc.tensor.matmul(ps[:, :c1 - c0], cv('ag1T'), x2[:, c0:c1], start=True, stop=True)
            nc.scalar.activation(g1[:, c0:c1], ps[:, :c1 - c0], F.Relu, bias=cv('ag1b'))
        gat = rot.tile([CH, L], BF16, tag="pb")
        for c0, c1 in MM_CHUNKS:
            ps = post_ps.tile([CH, 512], FP32, tag="ps")
            nc.tensor.matmul(ps[:, :c1 - c0], cv('ag2T'), g1[:, c0:c1], start=True, stop=True)
            nc.scalar.activation(gat[:, c0:c1], ps[:, :c1 - c0], F.Sigmoid, bias=cv('ag2b'))
        x2g = pP.tile([CH, L], BF16, tag="x2g")
        nc.vector.tensor_mul(x2g[:], x2[:], gat[:])

        zsA = pP.tile([HH, L], BF16, tag="zsA")
        zsB = pP.tile([HH, L], BF16, tag="zsB")
        nc.scalar.activation(zsA[:], z0[0:96, :], F.Silu)
        nc.scalar.activation(zsB[0:32, :], z0[96:128, :], F.Silu)
        nc.scalar.activation(zsB[32:64, :], z1[0:32, :], F.Silu)
        nc.scalar.activation(zsB[64:96, :], z1[32:64, :], F.Silu)

        ygA = pP.tile([HH, L], BF16, tag="ygA")
        ygB = pP.tile([HH, L], BF16, tag="ygB")
        nc.sync.dma_start(ygA[:], yg_dram[0:HH, :])
        nc.sync.dma_start(ygB[:], yg_dram[HH:DI, :])

        sA = pP.tile([1, L], FP32, tag="sA")
        sB = pP.tile([1, L], FP32, tag="sB")
        sM = pP.tile([1, L], FP32, tag="sM")

        def ln_stats(cinv):
            # in: sA=raw sum, sB=raw sumsq; leaves rstd in sB (sA stays raw sum)
            nc.scalar.activation(sM[:], sA[:], F.Square, scale=cinv)
            nc.vector.scalar_tensor_tensor(out=sB[:], in0=sB[:], scalar=cinv,
                                           in1=sM[:], op0=A.mult, op1=A.subtract)
            nc.vector.tensor_scalar(out=sB[:], in0=sB[:], scalar1=1e-5,
                                    scalar2=None, op0=A.add)
            nc.vector.reciprocal(sB[:], sB[:])
            nc.scalar.activation(sB[:], sB[:], F.Sqrt)

        # LayerNorm over full DI (local stats via ones-matmul)
        ysqA = rot.tile([HH, L], BF16, tag="pb")
        ysqB = rot.tile([HH, L], BF16, tag="pb")
        nc.vector.tensor_mul(ysqA[:], ygA[:], ygA[:])
        nc.vector.tensor_mul(ysqB[:], ygB[:], ygB[:])
        for c, (c0, c1) in enumerate(MM_CHUNKS):
            ps = post_ps.tile([1, 512], FP32, tag="ps")
            nc.tensor.matmul(ps[:, :c1 - c0], ones96[:], ygA[:, c0:c1], start=True, stop=False)
            nc.tensor.matmul(ps[:, :c1 - c0], ones96[:], ygB[:, c0:c1], start=False, stop=True)
            nc.scalar.copy(sA[0:1, c0:c1], ps[:, :c1 - c0])
            ps2 = post_ps.tile([1, 512], FP32, tag="ps")
            nc.tensor.matmul(ps2[:, :c1 - c0], ones96[:], ysqA[:, c0:c1], start=True, stop=False)
            nc.tensor.matmul(ps2[:, :c1 - c0], ones96[:], ysqB[:, c0:c1], start=False, stop=True)
            nc.scalar.copy(sB[0:1, c0:c1], ps2[:, :c1 - c0])

        ln_stats(1.0 / DI)

        def apply_ln(pairs, bco):
            # pairs: list of (dst, src, gname, bname); bco: bcv column offset
            # holding 1/DI or 1/CH (folds the mean division into the
            # broadcast lhsT)
            for c0, c1 in MM_CHUNKS:
                cw = c1 - c0
                psm = post_ps.tile([HH, 512], FP32, tag="ps")
                nc.tensor.matmul(psm[:, :cw], cvc('bcv', bco, bco + HH),
                                 sA[:, c0:c1], start=True, stop=True)
                psr = post_ps.tile([HH, 512], FP32, tag="ps")
                nc.tensor.matmul(psr[:, :cw], cvc('bcv', 192, 192 + HH),
                                 sB[:, c0:c1], start=True, stop=True)
                for (dst, srct, gname, bname) in pairs:
                    nc.vector.tensor_sub(dst[:, c0:c1], srct[:, c0:c1],
                                         psm[:, :cw])
                    nc.vector.tensor_mul(dst[:, c0:c1], dst[:, c0:c1],
                                         psr[:, :cw])
                    nc.vector.tensor_scalar(out=dst[:, c0:c1], in0=dst[:, c0:c1],
                                            scalar1=cv(gname), scalar2=cv(bname),
                                            op0=A.mult, op1=A.add)

        ynA = rot.tile([HH, L], BF16, tag="pb")
        ynB = rot.tile([HH, L], BF16, tag="pb")
        apply_ln([(ynA, ygA, 'outngA', 'outnbA'),
                  (ynB, ygB, 'outngB', 'outnbB')], 0)

        gzA = rot.tile([HH, L], BF16, tag="pb")
        gzB = rot.tile([HH, L], BF16, tag="pb")
        nc.vector.tensor_mul(gzA[:], ynA[:], zsA[:])
        nc.vector.tensor_mul(gzB[:], ynB[:], zsB[:])

        x1o = pP.tile([CH, L], BF16, tag="x1o")
        for c0, c1 in MM_CHUNKS:
            ps = post_ps.tile([CH, 512], FP32, tag="ps")
            nc.tensor.matmul(ps[:, :c1 - c0], cv('outwTa'), gzA[:, c0:c1],
                             start=True, stop=False)
            nc.tensor.matmul(ps[:, :c1 - c0], cv('outwTb'), gzB[:, c0:c1],
                             start=False, stop=True)
            nc.scalar.copy(x1o[:, c0:c1], ps[:, :c1 - c0])

        yb = pP.tile([CH, L], BF16, tag="yb")
        nc.vector.tensor_add(yb[:], x1o[:], x2g[:])

        # local LayerNorm over channels
        ybsq = rot.tile([CH, L], BF16, tag="pb")
        nc.vector.tensor_mul(ybsq[:], yb[:], yb[:])
        for c, (c0, c1) in enumerate(MM_CHUNKS):
            ps = post_ps.tile([1, 512], FP32, tag="ps")
            nc.tensor.matmul(ps[:, :c1 - c0], ones96[:], yb[:, c0:c1], start=True, stop=True)
            nc.scalar.copy(sA[0:1, c0:c1], ps[:, :c1 - c0])
            ps2 = post_ps.tile([1, 512], FP32, tag="ps")
            nc.tensor.matmul(ps2[:, :c1 - c0], ones96[:], ybsq[:, c0:c1], start=True, stop=True)
            nc.scalar.copy(sB[0:1, c0:c1], ps2[:, :c1 - c0])
        ln_stats(1.0 / CH)
        ybn = pP.tile([CH, L], BF16, tag="ybn")
        apply_ln([(ybn, yb, 'lng', 'lnb')], 96)

        # CRM
        low_t = rot.tile([48, L], BF16, tag="pb")
        nc.sync.dma_start(low_t[:], ybn[48:96, :])
        upc = pP.tile([24, L], BF16, tag="upc")
        lowc = pP.tile([24, L], BF16, tag="lowc")
        m2cb = pP.tile([24, 5], FP32, tag="m2cb")
        _li = {c0: i for i, (c0, c1) in enumerate(MM_CHUNKS)}.get
        for c0, c1 in MM_CHUNKS:
            ps = post_ps.tile([24, 512], FP32, tag="ps")
            nc.tensor.matmul(ps[:, :c1 - c0], cv('sq1T'), ybn[0:48, c0:c1], start=True, stop=True)
            nc.scalar.copy(upc[:, c0:c1], ps[:, :c1 - c0])
            ps2 = post_ps.tile([24, 512], FP32, tag="ps")
            nc.tensor.matmul(ps2[:, :c1 - c0], cv('sq2T'), low_t[:, c0:c1], start=True, stop=True)
            nc.scalar.activation(lowc[:, c0:c1], ps2[:, :c1 - c0], F.Identity,
                                 accum_out=m2cb[:, _li(c0):_li(c0) + 1])
        upcp = pP.tile([24, LP], BF16, tag="upcp")
        nc.gpsimd.memset(upcp[:], 0.0)
        nc.vector.tensor_copy(hwp(upcp[:])[:, 1:49, 1:49], hw(upc[:]))
        Y1 = pP.tile([CH, L], BF16, tag="Y1")
        m1c = pP.tile([CH, 5], FP32, tag="m1c")
        for ri, (r0, r1) in enumerate(ROW_CHUNKS):
            nr = r1 - r0
            ps = post_ps.tile([CH, 480], FP32, tag="ps")
            for tap in range(9):
                dy, dx = tap // 3, tap % 3
                rhs = hwp(upcp[:])[:, dy + r0:dy + r1, dx:dx + 48]
                nc.tensor.matmul(ps[:, :nr * 48], cvc('gwcT', tap * CH, (tap + 1) * CH),
                                 rhs, start=(tap == 0), stop=False)
            nc.tensor.matmul(ps[:, :nr * 48], cv('pw1T'), upc[:, r0 * 48:r1 * 48],
                             start=False, stop=True)
            nc.scalar.activation(Y1[:, r0 * 48:r1 * 48], ps[:, :nr * 48],
                                 F.Identity, bias=cv('gwcb'),
                                 accum_out=m1c[:, ri:ri + 1])
        Y2a = pP.tile([72, L], BF16, tag="Y2a")
        m2ca = pP.tile([72, 5], FP32, tag="m2ca")
        for ri, (c0, c1) in enumerate(MM_CHUNKS):
            ps = post_ps.tile([72, 512], FP32, tag="ps")
            nc.tensor.matmul(ps[:, :c1 - c0], cv('pw2T'), lowc[:, c0:c1], start=True, stop=True)
            nc.scalar.activation(Y2a[:, c0:c1], ps[:, :c1 - c0], F.Identity,
                                 accum_out=m2ca[:, ri:ri + 1])
        m1 = pP.tile([CH, 1], FP32, tag="m1")
        m2a_s = pP.tile([72, 1], FP32, tag="m2a_s")
        m2b_s = pP.tile([24, 1], FP32, tag="m2b_s")
        nc.vector.reduce_sum(m1[:], m1c[:], axis=mybir.AxisListType.X)
        nc.vector.reduce_sum(m2a_s[:], m2ca[:], axis=mybir.AxisListType.X)
        nc.vector.reduce_sum(m2b_s[:], m2cb[:], axis=mybir.AxisListType.X)
        smf = pP.tile([1, 2 * CH], FP32, tag="smf")
        nc.sync.dma_start(smf[0:1, 0:CH], m1[:, 0:1])
        nc.sync.dma_start(smf[0:1, CH:CH + 72], m2a_s[:, 0:1])
        nc.sync.dma_start(smf[0:1, CH + 72:2 * CH], m2b_s[:, 0:1])
        nc.vector.tensor_scalar(out=smf[:], in0=smf[:], scalar1=1.0 / L,
                                scalar2=None, op0=A.mult)
        mx = pP.tile([1, 1], FP32, tag="mx")
        nc.vector.reduce_max(mx[:], smf[:], axis=mybir.AxisListType.X)
        nc.vector.tensor_scalar(out=mx[:], in0=mx[:], scalar1=-1.0,
                                scalar2=None, op0=A.mult)
        nc.scalar.activation(smf[:], smf[:], F.Exp, bias=mx[0:1, 0:1])
        sm_s = pP.tile([1, 1], FP32, tag="sm_s")
        nc.vector.reduce_sum(sm_s[:], smf[:], axis=mybir.AxisListType.X)
        nc.vector.reciprocal(sm_s[:], sm_s[:])
        nc.vector.tensor_scalar(out=smf[:], in0=smf[:], scalar1=sm_s[0:1, 0:1],
                                scalar2=None, op0=A.mult)
        sm1 = pP.tile([CH, 1], FP32, tag="sm1")
        sm2 = pP.tile([CH, 1], FP32, tag="sm2")
        nc.sync.dma_start(sm1[:, 0:1], smf[0:1, 0:CH])
        nc.sync.dma_start(sm2[:, 0:1], smf[0:1, CH:2 * CH])
        o2f = rot.tile([CH, L], BF16, tag="pb")
        nc.sync.dma_start(o2f[0:72, :], Y2a[:])
        nc.sync.dma_start(o2f[72:96, :], lowc[:])
        o2t = pP.tile([CH, L], BF16, tag="o2t")
        nc.vector.tensor_scalar(out=o2t[:], in0=o2f[:], scalar1=sm2[:, 0:1],
                                scalar2=None, op0=A.mult)
        yc = pP.tile([CH, L], BF16, tag="yc")
        nc.vector.scalar_tensor_tensor(out=yc[:], in0=Y1[:], scalar=sm1[:, 0:1],
                                       in1=o2t[:], op0=A.mult, op1=A.add)
        outt = pP.tile([COUT, L], FP32, tag="outt")
        for c0, c1 in MM_CHUNKS:
            ps = post_ps.tile([COUT, 512], FP32, tag="ps")
            nc.tensor.matmul(ps[:, :c1 - c0], cv('finT'), yc[:, c0:c1], start=True, stop=True)
            nc.scalar.activation(outt[:, c0:c1], ps[:, :c1 - c0], F.Identity, bias=cv('finb'))
        nc.sync.dma_start(out_d[:], outt[:])
        po.close()
        glob.close()
    split_multi_waits(nc, max_waits=1)
    return nc


# =============================== host side ==================================

def prep_core_inputs(inputs, b, half):
    import ml_dtypes
    f32 = np.float32
    bf16 = ml_dtypes.bfloat16
    d0 = half * HH

    def asf(a):
        return np.asarray(a, f32)

    bnscale = asf(inputs['bn_g']) / np.sqrt(np.float32(1.0 + 1e-5))
    w1 = asf(inputs['conv1_w'])[:, :, 0, 0] * bnscale[:, None]
    b1 = asf(inputs['conv1_b']) * bnscale + asf(inputs['bn_b'])

    def diag9(w, nch):
        out = np.zeros((nch, 9 * nch), f32)
        w = asf(w)
        for tap in range(9):
            dy, dx = tap // 3, tap % 3
            blk = out[:, tap * nch:(tap + 1) * nch]
            np.fill_diagonal(blk, w[:, 0, dy, dx])
        return out

    sscd = diag9(inputs['ss_conv_w'], DI)        # (192, 9*192)
    sc0 = np.zeros((128, 9 * 128), f32)
    sc1 = np.zeros((64, 9 * 64), f32)
    for tap in range(9):
        blk = sscd[:, tap * DI:(tap + 1) * DI]
        sc0[:, tap * 128:(tap + 1) * 128] = blk[0:128, 0:128]
        sc1[:, tap * 64:(tap + 1) * 64] = blk[128:192, 128:192]

    sel = np.zeros((DI, HH), f32)
    sel[np.arange(d0, d0 + HH), np.arange(HH)] = 1.0

    xp = asf(inputs['ss_xproj_w'])               # (4, 38, 192)
    xpTa = np.zeros((128, K4 * 38), f32)
    xpTb = np.zeros((64, K4 * 38), f32)
    for k in range(K4):
        xpT = xp[k].T                            # (192, 38)
        xpTa[:, k * 38:(k + 1) * 38] = xpT[0:128]
        xpTb[:, k * 38:(k + 1) * 38] = xpT[128:192]

    dtw = asf(inputs['ss_dt_w'])
    dtwT = np.zeros((R, K4 * HH), f32)
    for k in range(K4):
        dtwT[:, k * HH:(k + 1) * HH] = dtw[k][d0:d0 + HH, :].T

    dtb_full = asf(inputs['ss_dt_b'])
    Alog = asf(inputs['ss_Alog']).reshape(K4, DI, N)
    Dv = asf(inputs['ss_D']).reshape(K4, DI)
    dtb_p = np.zeros((128, NT), f32)
    Ap = np.zeros((128, NT * N), f32)
    for (j, o0, o1, k, dd0, dd1) in SECTIONS:
        dtb_p[o0:o1, j] = dtb_full[k, d0 + dd0:d0 + dd1]
        Ap[o0:o1, j * N:(j + 1) * N] = -np.exp(Alog[k, d0 + dd0:d0 + dd1])
    Dsum = Dv[:, d0:d0 + HH].sum(0)[:, None]

    gw = asf(inputs['gwc_w'])
    gT = np.zeros((24, 9 * CH), f32)
    for tap in range(9):
        dy, dx = tap // 3, tap % 3
        blk = np.zeros((24, CH), f32)
        blk[0:12, 0:48] = gw[0:48, :, dy, dx].T
        blk[12:24, 48:96] = gw[48:96, :, dy, dx].T
        gT[:, tap * CH:(tap + 1) * CH] = blk

    owT = asf(inputs['ss_out_w']).T              # (192, 96)
    outn_g = asf(inputs['ss_outn_g'])
    outn_b = asf(inputs['ss_outn_b'])

    vals32 = {
        'w1T': w1.T, 'b1': b1[:, None],
        'linb': asf(inputs['lin_b'])[:, None],
        'dw1b': asf(inputs['dw1_b'])[:, None],
        'dw2b': asf(inputs['dw2_b'])[:, None],
        'scb0': asf(inputs['ss_conv_b'])[0:128, None],
        'scb1': asf(inputs['ss_conv_b'])[128:192, None],
        'dtb': dtb_p, 'Ap': Ap, 'Dsum': Dsum,
        'outngA': outn_g[0:96, None], 'outngB': outn_g[96:192, None],
        'outnbA': outn_b[0:96, None], 'outnbB': outn_b[96:192, None],
        'ag1b': asf(inputs['ag1_b'])[:, None],
        'ag2b': asf(inputs['ag2_b'])[:, None],
        'lng': asf(inputs['ln_g'])[:, None],
        'lnb': asf(inputs['ln_b'])[:, None],
        'gwcb': asf(inputs['gwc_b'])[:, None],
        'finb': asf(inputs['fin_b'])[:, None],
        'bcv': np.concatenate([np.full((1, 96), 1.0 / DI, f32),
                               np.full((1, 96), 1.0 / CH, f32),
                               np.ones((1, 96), f32)], axis=1),
    }
    valsbf = {
        'linT': asf(inputs['lin_w']).T,
        'dw1dg': diag9(inputs['dw1_w'], CH),
        'dw2dg': diag9(inputs['dw2_w'], CH),
        'inwT': asf(inputs['ss_in_w']).T,        # (96, 384) full z
        'sc0dg': sc0, 'sc1dg': sc1,
        'sel0': sel[0:128], 'sel1': sel[128:192],
        'xpTa': xpTa, 'xpTb': xpTb,
        'dtwT': dtwT,
        'ident': np.eye(128, dtype=f32),
        'outwTa': owT[0:96], 'outwTb': owT[96:192],
        'ag1T': asf(inputs['ag1_w'])[:, :, 0, 0].T,
        'ag2T': asf(inputs['ag2_w'])[:, :, 0, 0].T,
        'sq1T': asf(inputs['sq1_w'])[:, :, 0, 0].T,
        'sq2T': asf(inputs['sq2_w'])[:, :, 0, 0].T,
        'gwcT': gT,
        'pw1T': asf(inputs['pwc1_w'])[:, :, 0, 0].T,
        'pw2T': asf(inputs['pwc2_w'])[:, :, 0, 0].T,
        'finT': asf(inputs['fin_w']).T,
    }

    blob32 = np.zeros((128, W32), f32)
    for nm, p, c in CONSTS_F32:
        o = OFF32[nm][0]
        v = vals32[nm]
        assert v.shape == (p, c), (nm, v.shape, (p, c))
        blob32[0:p, o:o + c] = v
    blobbf = np.zeros((128, WBF), bf16)
    for nm, p, c in CONSTS_BF16:
        o = OFFBF[nm][0]
        v = valsbf[nm]
        assert v.shape == (p, c), (nm, v.shape, (p, c))
        blobbf[0:p, o:o + c] = v.astype(bf16)

    return {
        'x': np.ascontiguousarray(asf(inputs['x'])[b].reshape(CIN, L)),
        'c32': blob32,
        'cbf': np.ascontiguousarray(blobbf),
    }


_NC_CACHE = {}


def get_nc():
    if 'nc' not in _NC_CACHE:
        _NC_CACHE['nc'] = build_nc()
    return _NC_CACHE['nc']


def kernel(**inputs):
    from concourse.bass_utils import run_bass_kernel_spmd
    nc = get_nc()
    in_maps = [prep_core_inputs(inputs, c // 2, c % 2) for c in range(8)]
    res = run_bass_kernel_spmd(nc, in_maps, core_ids=list(range(8)))
    out = np.zeros((B_, COUT, H, W), np.float32)
    for b in range(B_):
        out[b] = res.results[2 * b]['out'].reshape(COUT, H, W)
    return out


# revision 57
# speedup vs baseline: 1.0328x; 1.0189x over previous
"""Self-contained Trainium2 Bass kernel for the CR-VSS block (8 cores)."""

# ---- TileContext drain-wait patch (walrus 1-wait limit) ----
"""Patch TileContext._drain_and_barrier: the axon-client walrus rejects
instructions carrying >2 sem waits ("Too many sync wait commands" in
setupSyncWait for CTRL structs). Redistribute the exit-drain's waits across
preceding SP nop instructions, each carrying at most MAX_WAITS."""
from concourse.tile import TileContext, ScopedClock

MAX_WAITS = 1


def _patched_drain_and_barrier(self, tick_clock, wait_clock):
    nc = self.nc
    drain_inst = nc.sync.drain()
    wait_clock.add_sem_waits(
        drain_inst.ins, ScopedClock({None: tick_clock.global_clock})
    )

    waits = list(drain_inst.ins.sync_info.on_wait or [])
    if len(waits) > MAX_WAITS:
        bb = nc.cur_bb.bb
        assert bb.instructions[-1] is drain_inst.ins
        # strip waits from the drain, re-emit them on nop carriers
        drain_inst.ins.sync_info.on_wait = waits[:0]
        carriers = []
        import concourse.mybir as mybir
        for i in range(0, len(waits), MAX_WAITS):
            nop = nc.sync.nop(nofuse=True)
            nop.ins.sync_info = mybir.SyncInfo(
                on_wait=waits[i:i + MAX_WAITS], on_update=[]
            )
            carriers.append(nop.ins)
        # move carriers before the drain
        insts = list(bb.instructions)
        assert insts[-len(carriers) - 1] is drain_inst.ins
        reordered = insts[:-len(carriers) - 1] + insts[-len(carriers):] + [drain_inst.ins]
        while len(bb.instructions):
            bb.instructions.pop()
        for x in reordered:
            bb.instructions.append(x)

    nc.all_engine_barrier()
    assert self.sems is not None
    popped = nc._tile_sem_poison_stack.pop()
    assert popped is self._sem_poison
    nc.clear_and_free_semaphores(list(self.sems.allocated().values()))
    nc.all_engine_barrier()


def apply():
    TileContext._drain_and_barrier = _patched_drain_and_barrier


def split_multi_waits(nc, max_waits=1):
    """Post-pass: walrus CTRL codegen rejects instructions with more than
    one sem wait. Move extra waits onto same-engine NoOp carriers."""
    import concourse.mybir as mybir
    for f in nc.m.functions:
        for bb in f.blocks:
            insts = list(bb.instructions)
            out = []
            changed = False
            for ins in insts:
                si = ins.sync_info
                if si is not None and si.on_wait and len(si.on_wait) > max_waits:
                    waits = list(si.on_wait)
                    for i, w in enumerate(waits[max_waits:]):
                        nop = mybir.InstNoOp.__new__(
                            mybir.InstNoOp, name=f"{ins.name}-xw{i}", ins=[], outs=[])
                        nop.engine = ins.engine
                        nop.sync_info = mybir.SyncInfo(on_wait=[w], on_update=[])
                        out.append(nop)
                    ins.sync_info = mybir.SyncInfo(
                        on_wait=waits[:max_waits],
                        on_update=list(si.on_update or []))
                    changed = True
                out.append(ins)
            if changed:
                while len(bb.instructions):
                    bb.instructions.pop()
                for x in out:
                    bb.instructions.append(x)

apply()

# ---- kernel ----
"""Trainium2 Bass kernel for nn_CR_VSS (VSS block with SS2D selective scan).

Sharding: 8 cores = 4 samples x 2 d_inner-halves. Each core runs the full
pre-stage for its sample, scans its 96-channel d-half across all 4
cross-scan directions (packed into 3x128-partition tiles), then the pair
exchanges y-halves with ONE AllGather; LN + out-proj + post-stage run
locally (z is computed full-width in the in-proj so no second collective).

Scan: h_t = exp(A*delta_t)*h_{t-1} + delta_t*u_t*B_t per (k,d,n) via
tensor_tensor_scan; n in groups of 4 with batched B/C partition-broadcast
DMAs (double-buffered); y accumulated over n with identity-lhsT PSUM
matmuls, merged into pixel-order ysum straight from PSUM per t-chunk.
"""
import numpy as np
from contextlib import ExitStack

import concourse.bass as bass
import concourse.mybir as mybir

F = mybir.ActivationFunctionType
A = mybir.AluOpType
FP32 = mybir.dt.float32
BF16 = mybir.dt.bfloat16

B_, CIN, CH, COUT, H, W = 4, 96, 96, 96, 48, 48
DI, N, R, K4 = 192, 16, 6, 4
L = H * W               # 2304
HH = 96                 # d-half per core
NT = 3                  # packed (k,d) tiles: 4*96 = 384 = 3*128
HP = 50
LP = 2500
TC = 768                # scan t-chunk (16 rows of 48)
TCH = [(0, 768), (768, 1536), (1536, 2304)]
NG = 4                  # scan n-group (broadcast batch)

# packed (k,d) rows -> (tile j, offset): sections (j, o0, o1, k, d0, d1).
# Section offsets are all 0/32/64 so PE matmuls can write them directly.
SECTIONS = [
    (0, 0, 32, 1, 0, 32),
    (0, 32, 128, 0, 0, 96),
    (1, 0, 64, 1, 32, 96),
    (1, 64, 128, 2, 0, 64),
    (2, 0, 32, 2, 64, 96),
    (2, 32, 128, 3, 0, 96),
]

MM_CHUNKS = [(0, 512), (512, 1024), (1024, 1536), (1536, 2048), (2048, 2304)]
ROW_CHUNKS = [(0, 10), (10, 20), (20, 30), (30, 40), (40, 48)]
SUBS768 = [(0, 512), (512, 768)]
INW_BLOCKS = [(0, 128), (128, 256), (256, 384)]

REPLICA_GROUPS = [[0, 1], [2, 3], [4, 5], [6, 7]]

# ---- const blobs (shared layout between host packing and kernel views) ----
CONSTS_F32 = [
    ('b1', 96, 1), ('linb', 96, 1),
    ('dw1b', 96, 1), ('dw2b', 96, 1),
    ('scb0', 128, 1), ('scb1', 64, 1),
    ('dtb', 128, 3), ('Ap', 128, 48), ('Dsum', 96, 1),
    ('outngA', 96, 1), ('outngB', 96, 1), ('outnbA', 96, 1), ('outnbB', 96, 1),
    ('ag1b', 48, 1), ('ag2b', 96, 1), ('lng', 96, 1), ('lnb', 96, 1),
    ('gwcb', 96, 1), ('finb', 96, 1), ('bcv', 1, 288),
]
CONSTS_BF16 = [
    ('w1T', 96, 96), ('linT', 96, 96),
    ('dw1dg', 96, 864), ('dw2dg', 96, 864),
    ('inwT', 96, 384),
    ('sc0dg', 128, 1152), ('sc1dg', 64, 576),
    ('sel0', 128, 96), ('sel1', 64, 96),
    ('xpTa', 128, 152), ('xpTb', 64, 152),
    ('dtwT', 6, 384),
    ('ident', 128, 128),
    ('outwTa', 96, 96), ('outwTb', 96, 96),
    ('ag1T', 96, 48), ('ag2T', 48, 96),
    ('sq1T', 48, 24), ('sq2T', 48, 24),
    ('gwcT', 24, 864), ('pw1T', 24, 96), ('pw2T', 24, 72),
    ('finT', 96, 96),
]

OFF32 = {}
_o = 0
for _nm, _p, _c in CONSTS_F32:
    OFF32[_nm] = (_o, _p, _c)
    _o += _c
W32 = _o
OFFBF = {}
_o = 0
for _nm, _p, _c in CONSTS_BF16:
    OFFBF[_nm] = (_o, _p, _c)
    _o += _c
WBF = _o


def build_nc():
    nc = bass.Bass(trn_type="TRN2", num_devices=8)

    x_d = nc.dram_tensor("x", [CIN, L], BF16, kind="ExternalInput")
    c32_d = nc.dram_tensor("c32", [128, W32], FP32, kind="ExternalInput")
    cbf_d = nc.dram_tensor("cbf", [128, WBF], BF16, kind="ExternalInput")
    out_d = nc.dram_tensor("out", [COUT, L], FP32, kind="ExternalOutput")

    B_dram = nc.dram_tensor("B_dram", [K4 * N, L], BF16)
    C_dram = nc.dram_tensor("C_dram", [K4 * N, L], BF16)
    y_dram = nc.dram_tensor("y_dram", [HH, L], BF16)
    yg_dram = nc.dram_tensor("yg_dram", [DI, L], BF16)
    st_dram = nc.dram_tensor("st_dram", [2, L], BF16)

    def hw(ap):
        return ap.rearrange("p (h w) -> p h w", h=H)

    def hwp(ap):
        return ap.rearrange("p (h w) -> p h w", h=HP)

    def whv(ap):
        return ap.rearrange("p (h w) -> p w h", h=H)

    with TileContext(nc) as tc:
        glob = ExitStack()
        cst = glob.enter_context(tc.tile_pool(name="cst", bufs=1))
        lngA = glob.enter_context(tc.tile_pool(name="lngA", bufs=1))

        cst32 = cst.tile([128, W32], FP32, tag="cst32")
        cstbf = cst.tile([128, WBF], BF16, tag="cstbf")
        nc.sync.dma_start(cst32[:], c32_d[:])
        nc.sync.dma_start(cstbf[:], cbf_d[:])

        def cvc(nm, a0=0, a1=None, p0=0, p1=None):
            d, tile = (OFF32, cst32) if nm in OFF32 else (OFFBF, cstbf)
            o, p, c = d[nm]
            if a1 is None:
                a1 = c
            if p1 is None:
                p1 = p
            return tile[p0:p1, o + a0:o + a1]

        cv = cvc

        ones96 = cst.tile([HH, 1], BF16, tag="ones96")
        nc.vector.memset(ones96[:], 1.0)

        # long-lived across phases
        z0 = lngA.tile([128, L], BF16, tag="z0")     # z rows 0:128
        z1 = lngA.tile([64, L], BF16, tag="z1")      # z rows 128:192
        x2 = lngA.tile([CH, L], BF16, tag="x2")
        lngB = ExitStack()
        lngB_p = lngB.enter_context(tc.tile_pool(name="lngB_p", bufs=1))
        xch = lngB_p.tile([HH, L], BF16, tag="xch")
        dp = [lngB_p.tile([128, L], BF16, tag=f"dp{j}", name=f"dp{j}") for j in range(NT)]
        # xsp holds packed scan-order xs, overwritten in place with delta*u
        xsp = [lngB_p.tile([128, L], BF16, tag=f"xsp{j}", name=f"xsp{j}") for j in range(NT)]
        ysum = lngB_p.tile([HH, L], BF16, tag="ysum")

        # ================= pre-stage =================
        pre = ExitStack()
        pre_ps = pre.enter_context(tc.tile_pool(name="pre_ps", bufs=4, space="PSUM"))
        pA = pre.enter_context(tc.tile_pool(name="pA", bufs=1))
        pB = pre.enter_context(tc.tile_pool(name="pB", bufs=1))

        xt = pA.tile([CIN, L], BF16, tag="xt")
        nc.sync.dma_start(xt[:], x_d[:])

        # conv1x1 (+folded BN) + ReLU
        h1 = pA.tile([CH, L], BF16, tag="h1")
        for c0, c1 in MM_CHUNKS:
            ps = pre_ps.tile([CH, 512], FP32, tag="ps")
            nc.tensor.matmul(ps[:, :c1 - c0], cv('w1T'), xt[:, c0:c1], start=True, stop=True)
            nc.scalar.activation(h1[:, c0:c1], ps[:, :c1 - c0], F.Relu, bias=cv('b1'))
        # token linear
        h2 = pA.tile([CH, L], BF16, tag="h2")
        for c0, c1 in MM_CHUNKS:
            ps = pre_ps.tile([CH, 512], FP32, tag="ps")
            nc.tensor.matmul(ps[:, :c1 - c0], cv('linT'), h1[:, c0:c1], start=True, stop=True)
            nc.vector.tensor_scalar(out=h2[:, c0:c1], in0=ps[:, :c1 - c0],
                                    scalar1=cv('linb'), scalar2=None, op0=A.add)
        h2p = pA.tile([CH, LP], BF16, tag="h2p")
        nc.gpsimd.memset(h2p[:], 0.0)
        for (r0, r1) in ROW_CHUNKS:
            nc.vector.tensor_copy(hwp(h2p[:])[:, r0 + 1:r1 + 1, 1:49],
                                  hw(h2[:])[:, r0:r1, :])

        def dwconv(dst, src_p, dgname, biasname, nch):
            for (r0, r1) in ROW_CHUNKS:
                nr = r1 - r0
                ps = pre_ps.tile([128, 480], FP32, tag="ps")
                for tap in range(9):
                    dy, dx = tap // 3, tap % 3
                    rhs = hwp(src_p[:])[:, dy + r0:dy + r1, dx:dx + 48]
                    nc.tensor.matmul(ps[:nch, :nr * 48],
                                     cvc(dgname, tap * nch, (tap + 1) * nch),
                                     rhs, start=(tap == 0), stop=(tap == 8))
                nc.scalar.activation(dst[:, r0 * 48:r1 * 48], ps[:nch, :nr * 48],
                                     F.Silu, bias=cv(biasname))

        x1 = pB.tile([CH, L], BF16, tag="x1")
        dwconv(x1, h2p, 'dw1dg', 'dw1b', CH)

        # in-proj: xi (192) + FULL z (192)
        xi0 = pB.tile([128, L], BF16, tag="xi0")
        xi1 = pB.tile([64, L], BF16, tag="xi1")
        for mb, (m0, m1) in enumerate(INW_BLOCKS):
            for c0, c1 in MM_CHUNKS:
                ps = pre_ps.tile([128, 512], FP32, tag="ps")
                nc.tensor.matmul(ps[:m1 - m0, :c1 - c0], cvc('inwT', m0, m1),
                                 x1[:, c0:c1], start=True, stop=True)
                if mb == 0:
                    nc.vector.tensor_copy(xi0[:, c0:c1], ps[:128, :c1 - c0])
                elif mb == 1:
                    nc.scalar.copy(xi1[:, c0:c1], ps[0:64, :c1 - c0])
                    nc.scalar.copy(z0[0:64, c0:c1], ps[64:128, :c1 - c0])
                else:
                    nc.scalar.copy(z0[64:128, c0:c1], ps[0:64, :c1 - c0])
                    nc.scalar.copy(z1[0:64, c0:c1], ps[64:128, :c1 - c0])

        xi0p = pB.tile([128, LP], BF16, tag="xi0p")
        xi1p = pB.tile([64, LP], BF16, tag="xi1p")
        nc.gpsimd.memset(xi0p[:], 0.0)
        nc.gpsimd.memset(xi1p[:], 0.0)
        for (r0, r1) in ROW_CHUNKS:
            nc.vector.tensor_copy(hwp(xi0p[:])[:, r0 + 1:r1 + 1, 1:49],
                                  hw(xi0[:])[:, r0:r1, :])
            nc.vector.tensor_copy(hwp(xi1p[:])[:, r0 + 1:r1 + 1, 1:49],
                                  hw(xi1[:])[:, r0:r1, :])
        xc0 = pB.tile([128, L], BF16, tag="xc0")
        xc1 = pB.tile([64, L], BF16, tag="xc1")
        dwconv(xc0, xi0p, 'sc0dg', 'scb0', 128)
        dwconv(xc1, xi1p, 'sc1dg', 'scb1', 64)

        # d-half extraction + wh copy
        for c0, c1 in MM_CHUNKS:
            ps = pre_ps.tile([HH, 512], FP32, tag="ps")
            nc.tensor.matmul(ps[:, :c1 - c0], cv('sel0'), xc0[:, c0:c1], start=True, stop=False)
            nc.tensor.matmul(ps[:, :c1 - c0], cv('sel1'), xc1[:, c0:c1], start=False, stop=True)
            nc.vector.tensor_copy(xch[:, c0:c1], ps[:, :c1 - c0])
        xwhh = pB.tile([HH, L], BF16, tag="xwhh")
        for (t0, t1) in TCH:
            w0, w1 = t0 // 48, t1 // 48
            nc.vector.tensor_copy(hw(xwhh[:])[:, w0:w1, :],
                                  whv(xch[:])[:, w0:w1, :])

        # xproj (compact 38 rows: 0:6 dts, 6:22 B, 22:38 C) in scan order
        def xc_read(k, c0, c1):
            if k == 0:
                return (xc0[:, c0:c1], xc1[:, c0:c1])
            if k == 1:
                return (whv(xc0[:])[:, c0 // 48:c1 // 48, :],
                        whv(xc1[:])[:, c0 // 48:c1 // 48, :])
            if k == 2:
                return (xc0[:, L - c1:L - c0][:, ::-1],
                        xc1[:, L - c1:L - c0][:, ::-1])
            r0 = whv(xc0[:])[:, (L - c1) // 48:(L - c0) // 48, :][:, ::-1, ::-1]
            r1 = whv(xc1[:])[:, (L - c1) // 48:(L - c0) // 48, :][:, ::-1, ::-1]
            return (r0, r1)

        # row-chunk outer so all 4 directions' early columns finish first;
        # B/C are written to DRAM per scan chunk so ci=0 broadcasts can
        # start while xproj still works on later chunks.
        stage = [pB.tile([38, L], BF16, tag=f"stg{k}", name=f"stg{k}") for k in range(K4)]
        done_w = 0
        for ri, (rr0, rr1) in enumerate(ROW_CHUNKS):
            c0, c1 = rr0 * 48, rr1 * 48
            nf = c1 - c0
            for k in range(K4):
                ra, rb = xc_read(k, c0, c1)
                ps = pre_ps.tile([38, 480], FP32, tag="ps")
                nc.tensor.matmul(ps[:, :nf], cvc('xpTa', k * 38, (k + 1) * 38), ra,
                                 start=True, stop=False)
                nc.tensor.matmul(ps[:, :nf], cvc('xpTb', k * 38, (k + 1) * 38), rb,
                                 start=False, stop=True)
                nc.vector.tensor_copy(stage[k][:, c0:c1], ps[:, :nf])
            while done_w < len(TCH) and TCH[done_w][1] <= c1:
                t0, t1 = TCH[done_w]
                for k in range(K4):
                    nc.sync.dma_start(B_dram[k * N:(k + 1) * N, t0:t1],
                                      stage[k][6:22, t0:t1])
                    nc.sync.dma_start(C_dram[k * N:(k + 1) * N, t0:t1],
                                      stage[k][22:38, t0:t1])
                done_w += 1

        # delta: packed matmuls then softplus on full 128-partition tiles
        def mm_windows(a0, a1):
            if a0 == 0:
                return [(0, a1)]
            res = []
            x = a0
            while x < a1:
                if x % 64 == 32:
                    e = min(a1, x + 32)
                else:  # x == 64
                    e = min(a1, 128)
                res.append((x, e))
                x = e
            return res

        for (cc0, cc1) in MM_CHUNKS:
            cw = cc1 - cc0
            for j in range(NT):
                ex = pre_ps.tile([128, 512], FP32, tag="ps")
                for (jj, o0, o1, k, d0, d1) in SECTIONS:
                    if jj != j:
                        continue
                    for (w0, w1) in mm_windows(o0, o1):
                        dd0 = d0 + (w0 - o0)
                        dd1 = d0 + (w1 - o0)
                        nc.tensor.matmul(ex[w0:w1, :cw],
                                         cvc('dtwT', k * 96 + dd0, k * 96 + dd1),
                                         stage[k][0:6, cc0:cc1], start=True, stop=True)
                # softplus(x+b) = ln(1 + exp(x+b)) (no softplus act table on HW)
                ex2 = pre_ps.tile([128, 512], FP32, tag="ps")
                nc.scalar.activation(ex2[:, :cw], ex[:, :cw], F.Exp,
                                     bias=cvc('dtb', j, j + 1))
                nc.scalar.activation(dp[j][:, cc0:cc1], ex2[:, :cw], F.Ln, bias=1.0)

        # pack scan-order xs (Act copies handle partition shift + flips),
        # then overwrite in place with delta*u = dp*xs.
        # Act partition windows must not cross engine block boundaries on
        # EITHER side: allowed starts 0/32/64/96; a start-32 window may not
        # cross 64. split2 chops a shifted copy accordingly.
        def _legal_span(s):
            return 32 if s == 32 else 128 - s if s else 128

        def split2(o0, i0, ln):
            res = []
            x = 0
            while x < ln:
                step = min(ln - x, _legal_span(o0 + x), _legal_span(i0 + x))
                res.append((x, x + step))
                x += step
            return res

        for (t0, t1) in TCH:
            for (j, o0, o1, k, d0, d1) in SECTIONS:
                v = xwhh if k in (1, 3) else xch
                if k < 2:
                    # forward sections: contiguous rows, cheap DMA shift
                    nc.sync.dma_start(xsp[j][o0:o1, t0:t1], v[d0:d1, t0:t1])
                    continue
                for (w0, w1) in split2(o0, d0, o1 - o0):
                    nc.scalar.copy(xsp[j][o0 + w0:o0 + w1, t0:t1],
                                   v[d0 + w0:d0 + w1, ::-1][:, t0:t1])
            for j in range(NT):
                nc.vector.tensor_mul(xsp[j][:, t0:t1], dp[j][:, t0:t1],
                                     xsp[j][:, t0:t1])

        # branch-2 dwconv last: x2 is only needed in the post stage, so
        # this fills PE/Act slack once the scan inputs are queued
        dwconv(x2, h2p, 'dw2dg', 'dw2b', CH)

        pre.close()

        # ================= scan =================
        sc = ExitStack()
        bbp = sc.enter_context(tc.tile_pool(name="bbp", bufs=2))
        spool = sc.enter_context(tc.tile_pool(name="spool", bufs=2))
        scan_ps = sc.enter_context(tc.tile_pool(name="scan_ps", bufs=1, space="PSUM"))
        stp = sc.enter_context(tc.tile_pool(name="stp", bufs=1))
        state = [stp.tile([128, N], FP32, tag=f"st{j}", name=f"state{j}")
                 for j in range(NT)]
        yd = [stp.tile([HH, L], BF16, tag=f"yd{k}", name=f"yd{k}") for k in range(K4)]

        pending_drain = None
        for ci, (c0, c1) in enumerate(TCH):
            ypsum = [scan_ps.tile([128, TC], FP32, tag=f"yps{j}", name=f"yps{j}_{ci}")
                     for j in range(NT)]
            for g in range(K4):
                Bb = [bbp.tile([128, NG * TC], BF16, tag=f"Bb{j}", name=f"Bb{j}_{ci}_{g}")
                      for j in range(NT)]
                Cb = [bbp.tile([128, NG * TC], BF16, tag=f"Cb{j}", name=f"Cb{j}_{ci}_{g}")
                      for j in range(NT)]
                for (j, o0, o1, k, d0, d1) in SECTIONS:
                    nc.sync.dma_start(
                        Bb[j][o0:o1, :],
                        B_dram[k * N + NG * g:k * N + NG * g + NG, c0:c1]
                        .partition_broadcast(o1 - o0))
                    nc.scalar.dma_start(
                        Cb[j][o0:o1, :],
                        C_dram[k * N + NG * g:k * N + NG * g + NG, c0:c1]
                        .partition_broadcast(o1 - o0))
                if pending_drain is not None:
                    pending_drain()
                    pending_drain = None
                for n4 in range(NG):
                    n = NG * g + n4
                    for j in range(NT):
                        at = spool.tile([128, TC], BF16, tag=f"at{j}", name=f"at{j}")
                        nc.scalar.activation(at[:], dp[j][:, c0:c1], F.Exp,
                                             scale=cvc('Ap', j * N + n, j * N + n + 1))
                        bt = spool.tile([128, TC], BF16, tag=f"bt{j}", name=f"bt{j}")
                        nc.vector.tensor_mul(bt[:], xsp[j][:, c0:c1],
                                             Bb[j][:, n4 * TC:(n4 + 1) * TC])
                        ht = spool.tile([128, TC], BF16, tag=f"ht{j}", name=f"ht{j}")
                        if ci > 0:
                            # fold carry state into bt[0] so the scan can use
                            # the cheap zero-init form
                            nc.vector.scalar_tensor_tensor(
                                out=bt[:, 0:1], in0=at[:, 0:1],
                                scalar=state[j][:, n:n + 1], in1=bt[:, 0:1],
                                op0=A.mult, op1=A.add)
                        nc.vector.tensor_tensor_scan(ht[:], at[:], bt[:], 0.0,
                                                     A.mult, A.add)
                        if ci < 2:
                            nc.vector.tensor_copy(state[j][:, n:n + 1], ht[:, TC - 1:TC])
                        gt = spool.tile([128, TC], BF16, tag=f"gt{j}", name=f"gt{j}")
                        nc.vector.tensor_mul(gt[:], ht[:],
                                             Cb[j][:, n4 * TC:(n4 + 1) * TC])
                        for (s0, s1) in SUBS768:
                            nc.tensor.matmul(ypsum[j][:, s0:s1], cv('ident'),
                                             gt[:, s0:s1],
                                             start=(n == 0), stop=(n == N - 1))
            # drain this chunk's PSUM into per-direction scan-order tiles
            # (Act copies allow the partition shift). Deferred past the next
            # chunk's broadcast issue so the boundary doesn't stall Act.
            def _drain(yps=ypsum, cc0=c0, cc1=c1):
                for (j, o0, o1, k, d0, d1) in SECTIONS:
                    for (w0, w1) in split2(d0, o0, d1 - d0):
                        nc.scalar.copy(yd[k][d0 + w0:d0 + w1, cc0:cc1],
                                       yps[j][o0 + w0:o0 + w1, :])
            pending_drain = _drain
        if pending_drain is not None:
            pending_drain()
            pending_drain = None
        # merge directions into pixel order + D*u term
        tmp96 = stp.tile([HH, L], BF16, tag="tmp96")
        nc.vector.tensor_add(ysum[:], yd[0][:], yd[2][:, ::-1])
        nc.vector.tensor_add(tmp96[:], yd[1][:], yd[3][:, ::-1])
        nc.vector.tensor_add(ysum[:], ysum[:], whv(tmp96[:]))
        nc.vector.scalar_tensor_tensor(out=ysum[:], in0=xch[:], scalar=cv('Dsum'),
                                       in1=ysum[:], op0=A.mult, op1=A.add)
        nc.sync.dma_start(y_dram[:], ysum[:])
        nc.gpsimd.collective_compute(
            "AllGather", A.bypass, replica_groups=REPLICA_GROUPS,
            ins=[y_dram[:]], outs=[yg_dram[:]])
        sc.close()
        lngB.close()

        # ================= post =================
        po = ExitStack()
        post_ps = po.enter_context(tc.tile_pool(name="post_ps", bufs=4, space="PSUM"))
        pP = po.enter_context(tc.tile_pool(name="pP", bufs=1))
        rot = po.enter_context(tc.tile_pool(name="rot", bufs=4))

        # branch 2 + silu(z): no dep on the collective, runs under it
        g1 = rot.tile([48, L], BF16, tag="pb")
        for c0, c1 in MM_CHUNKS:
            ps = post_ps.tile([48, 512], FP32, tag="ps")
            nc.tensor.matmul(ps[:, :c1 - c0], cv('ag1T'), x2[:, c0:c1], start=True, stop=True)
            nc.scalar.activation(g1[:, c0:c1], ps[:, :c1 - c0], F.Relu, bias=cv('ag1b'))
        gat = rot.tile([CH, L], BF16, tag="pb")
        for c0, c1 in MM_CHUNKS:
            ps = post_ps.tile([CH, 512], FP32, tag="ps")
            nc.tensor.matmul(ps[:, :c1 - c0], cv('ag2T'), g1[:, c0:c1], start=True, stop=True)
            nc.scalar.activation(gat[:, c0:c1], ps[:, :c1 - c0], F.Sigmoid, bias=cv('ag2b'))
        x2g = pP.tile([CH, L], BF16, tag="x2g")
        nc.vector.tensor_mul(x2g[:], x2[:], gat[:])

        zsA = pP.tile([HH, L], BF16, tag="zsA")
        zsB = pP.tile([HH, L], BF16, tag="zsB")
        nc.scalar.activation(zsA[:], z0[0:96, :], F.Silu)
        nc.scalar.activation(zsB[0:32, :], z0[96:128, :], F.Silu)
        nc.scalar.activation(zsB[32:64, :], z1[0:32, :], F.Silu)
        nc.scalar.activation(zsB[64:96, :], z1[32:64, :], F.Silu)

        ygA = pP.tile([HH, L], BF16, tag="ygA")
        ygB = pP.tile([HH, L], BF16, tag="ygB")
        nc.sync.dma_start(ygA[:], yg_dram[0:HH, :])
        nc.sync.dma_start(ygB[:], yg_dram[HH:DI, :])

        sA = pP.tile([1, L], FP32, tag="sA")
        sB = pP.tile([1, L], FP32, tag="sB")
        sM = pP.tile([1, L], FP32, tag="sM")

        def ln_stats(cinv):
            # in: sA=raw sum, sB=raw sumsq; leaves rstd in sB (sA stays raw sum)
            nc.scalar.activation(sM[:], sA[:], F.Square, scale=cinv)
            nc.vector.scalar_tensor_tensor(out=sB[:], in0=sB[:], scalar=cinv,
                                           in1=sM[:], op0=A.mult, op1=A.subtract)
            nc.vector.tensor_scalar(out=sB[:], in0=sB[:], scalar1=1e-5,
                                    scalar2=None, op0=A.add)
            nc.vector.reciprocal(sB[:], sB[:])
            nc.scalar.activation(sB[:], sB[:], F.Sqrt)

        # LayerNorm over full DI (local stats via ones-matmul)
        ysqA = rot.tile([HH, L], BF16, tag="pb")
        ysqB = rot.tile([HH, L], BF16, tag="pb")
        nc.vector.tensor_mul(ysqA[:], ygA[:], ygA[:])
        nc.vector.tensor_mul(ysqB[:], ygB[:], ygB[:])
        for c, (c0, c1) in enumerate(MM_CHUNKS):
            ps = post_ps.tile([1, 512], FP32, tag="ps")
            nc.tensor.matmul(ps[:, :c1 - c0], ones96[:], ygA[:, c0:c1], start=True, stop=False)
            nc.tensor.matmul(ps[:, :c1 - c0], ones96[:], ygB[:, c0:c1], start=False, stop=True)
            nc.scalar.copy(sA[0:1, c0:c1], ps[:, :c1 - c0])
            ps2 = post_ps.tile([1, 512], FP32, tag="ps")
            nc.tensor.matmul(ps2[:, :c1 - c0], ones96[:], ysqA[:, c0:c1], start=True, stop=False)
            nc.tensor.matmul(ps2[:, :c1 - c0], ones96[:], ysqB[:, c0:c1], start=False, stop=True)
            nc.scalar.copy(sB[0:1, c0:c1], ps2[:, :c1 - c0])

        ln_stats(1.0 / DI)

        def apply_ln(pairs, bco):
            # pairs: list of (dst, src, gname, bname); bco: bcv column offset
            # holding 1/DI or 1/CH (folds the mean division into the
            # broadcast lhsT)
            for c0, c1 in MM_CHUNKS:
                cw = c1 - c0
                psm = post_ps.tile([HH, 512], FP32, tag="ps")
                nc.tensor.matmul(psm[:, :cw], cvc('bcv', bco, bco + HH),
                                 sA[:, c0:c1], start=True, stop=True)
                psr = post_ps.tile([HH, 512], FP32, tag="ps")
                nc.tensor.matmul(psr[:, :cw], cvc('bcv', 192, 192 + HH),
                                 sB[:, c0:c1], start=True, stop=True)
                for (dst, srct, gname, bname) in pairs:
                    nc.vector.tensor_sub(dst[:, c0:c1], srct[:, c0:c1],
                                         psm[:, :cw])
                    nc.vector.tensor_mul(dst[:, c0:c1], dst[:, c0:c1],
                                         psr[:, :cw])
                    nc.vector.tensor_scalar(out=dst[:, c0:c1], in0=dst[:, c0:c1],
                                            scalar1=cv(gname), scalar2=cv(bname),
                                            op0=A.mult, op1=A.add)

        ynA = rot.tile([HH, L], BF16, tag="pb")
        ynB = rot.tile([HH, L], BF16, tag="pb")
        apply_ln([(ynA, ygA, 'outngA', 'outnbA'),
                  (ynB, ygB, 'outngB', 'outnbB')], 0)

        gzA = rot.tile([HH, L], BF16, tag="pb")
        gzB = rot.tile([HH, L], BF16, tag="pb")
        nc.vector.tensor_mul(gzA[:], ynA[:], zsA[:])
        nc.vector.tensor_mul(gzB[:], ynB[:], zsB[:])

        x1o = pP.tile([CH, L], BF16, tag="x1o")
        for c0, c1 in MM_CHUNKS:
            ps = post_ps.tile([CH, 512], FP32, tag="ps")
            nc.tensor.matmul(ps[:, :c1 - c0], cv('outwTa'), gzA[:, c0:c1],
                             start=True, stop=False)
            nc.tensor.matmul(ps[:, :c1 - c0], cv('outwTb'), gzB[:, c0:c1],
                             start=False, stop=True)
            nc.scalar.copy(x1o[:, c0:c1], ps[:, :c1 - c0])

        yb = pP.tile([CH, L], BF16, tag="yb")
        nc.vector.tensor_add(yb[:], x1o[:], x2g[:])

        # local LayerNorm over channels
        ybsq = rot.tile([CH, L], BF16, tag="pb")
        nc.vector.tensor_mul(ybsq[:], yb[:], yb[:])
        for c, (c0, c1) in enumerate(MM_CHUNKS):
            ps = post_ps.tile([1, 512], FP32, tag="ps")
            nc.tensor.matmul(ps[:, :c1 - c0], ones96[:], yb[:, c0:c1], start=True, stop=True)
            nc.scalar.copy(sA[0:1, c0:c1], ps[:, :c1 - c0])
            ps2 = post_ps.tile([1, 512], FP32, tag="ps")
            nc.tensor.matmul(ps2[:, :c1 - c0], ones96[:], ybsq[:, c0:c1], start=True, stop=True)
            nc.scalar.copy(sB[0:1, c0:c1], ps2[:, :c1 - c0])
        ln_stats(1.0 / CH)
        ybn = pP.tile([CH, L], BF16, tag="ybn")
        apply_ln([(ybn, yb, 'lng', 'lnb')], 96)

        # CRM
        low_t = rot.tile([48, L], BF16, tag="pb")
        nc.sync.dma_start(low_t[:], ybn[48:96, :])
        upc = pP.tile([24, L], BF16, tag="upc")
        lowc = pP.tile([24, L], BF16, tag="lowc")
        m2cb = pP.tile([24, 5], FP32, tag="m2cb")
        _li = {c0: i for i, (c0, c1) in enumerate(MM_CHUNKS)}.get
        for c0, c1 in MM_CHUNKS:
            ps = post_ps.tile([24, 512], FP32, tag="ps")
            nc.tensor.matmul(ps[:, :c1 - c0], cv('sq1T'), ybn[0:48, c0:c1], start=True, stop=True)
            nc.scalar.copy(upc[:, c0:c1], ps[:, :c1 - c0])
            ps2 = post_ps.tile([24, 512], FP32, tag="ps")
            nc.tensor.matmul(ps2[:, :c1 - c0], cv('sq2T'), low_t[:, c0:c1], start=True, stop=True)
            nc.scalar.activation(lowc[:, c0:c1], ps2[:, :c1 - c0], F.Identity,
                                 accum_out=m2cb[:, _li(c0):_li(c0) + 1])
        upcp = pP.tile([24, LP], BF16, tag="upcp")
        nc.gpsimd.memset(upcp[:], 0.0)
        nc.vector.tensor_copy(hwp(upcp[:])[:, 1:49, 1:49], hw(upc[:]))
        Y1 = pP.tile([CH, L], BF16, tag="Y1")
        m1c = pP.tile([CH, 5], FP32, tag="m1c")
        for ri, (r0, r1) in enumerate(ROW_CHUNKS):
            nr = r1 - r0
            ps = post_ps.tile([CH, 480], FP32, tag="ps")
            for tap in range(9):
                dy, dx = tap // 3, tap % 3
                rhs = hwp(upcp[:])[:, dy + r0:dy + r1, dx:dx + 48]
                nc.tensor.matmul(ps[:, :nr * 48], cvc('gwcT', tap * CH, (tap + 1) * CH),
                                 rhs, start=(tap == 0), stop=False)
            nc.tensor.matmul(ps[:, :nr * 48], cv('pw1T'), upc[:, r0 * 48:r1 * 48],
                             start=False, stop=True)
            nc.scalar.activation(Y1[:, r0 * 48:r1 * 48], ps[:, :nr * 48],
                                 F.Identity, bias=cv('gwcb'),
                                 accum_out=m1c[:, ri:ri + 1])
        Y2a = pP.tile([72, L], BF16, tag="Y2a")
        m2ca = pP.tile([72, 5], FP32, tag="m2ca")
        for ri, (c0, c1) in enumerate(MM_CHUNKS):
            ps = post_ps.tile([72, 512], FP32, tag="ps")
            nc.tensor.matmul(ps[:, :c1 - c0], cv('pw2T'), lowc[:, c0:c1], start=True, stop=True)
            nc.scalar.activation(Y2a[:, c0:c1], ps[:, :c1 - c0], F.Identity,
                                 accum_out=m2ca[:, ri:ri + 1])
        m1 = pP.tile([CH, 1], FP32, tag="m1")
        m2a_s = pP.tile([72, 1], FP32, tag="m2a_s")
        m2b_s = pP.tile([24, 1], FP32, tag="m2b_s")
        nc.vector.reduce_sum(m1[:], m1c[:], axis=mybir.AxisListType.X)
        nc.vector.reduce_sum(m2a_s[:], m2ca[:], axis=mybir.AxisListType.X)
        nc.vector.reduce_sum(m2b_s[:], m2cb[:], axis=mybir.AxisListType.X)
        smf = pP.tile([1, 2 * CH], FP32, tag="smf")
        nc.sync.dma_start(smf[0:1, 0:CH], m1[:, 0:1])
        nc.sync.dma_start(smf[0:1, CH:CH + 72], m2a_s[:, 0:1])
        nc.sync.dma_start(smf[0:1, CH + 72:2 * CH], m2b_s[:, 0:1])
        nc.vector.tensor_scalar(out=smf[:], in0=smf[:], scalar1=1.0 / L,
                                scalar2=None, op0=A.mult)
        mx = pP.tile([1, 1], FP32, tag="mx")
        nc.vector.reduce_max(mx[:], smf[:], axis=mybir.AxisListType.X)
        nc.vector.tensor_scalar(out=mx[:], in0=mx[:], scalar1=-1.0,
                                scalar2=None, op0=A.mult)
        nc.scalar.activation(smf[:], smf[:], F.Exp, bias=mx[0:1, 0:1])
        sm_s = pP.tile([1, 1], FP32, tag="sm_s")
        nc.vector.reduce_sum(sm_s[:], smf[:], axis=mybir.AxisListType.X)
        nc.vector.reciprocal(sm_s[:], sm_s[:])
        nc.vector.tensor_scalar(out=smf[:], in0=smf[:], scalar1=sm_s[0:1, 0:1],
                                scalar2=None, op0=A.mult)
        sm1 = pP.tile([CH, 1], FP32, tag="sm1")
        sm2 = pP.tile([CH, 1], FP32, tag="sm2")
        nc.sync.dma_start(sm1[:, 0:1], smf[0:1, 0:CH])
        nc.sync.dma_start(sm2[:, 0:1], smf[0:1, CH:2 * CH])
        o2f = rot.tile([CH, L], BF16, tag="pb")
        nc.sync.dma_start(o2f[0:72, :], Y2a[:])
        nc.sync.dma_start(o2f[72:96, :], lowc[:])
        o2t = pP.tile([CH, L], BF16, tag="o2t")
        nc.vector.tensor_scalar(out=o2t[:], in0=o2f[:], scalar1=sm2[:, 0:1],
                                scalar2=None, op0=A.mult)
        yc = pP.tile([CH, L], BF16, tag="yc")
        nc.vector.scalar_tensor_tensor(out=yc[:], in0=Y1[:], scalar=sm1[:, 0:1],
                                       in1=o2t[:], op0=A.mult, op1=A.add)
        outt = pP.tile([COUT, L], FP32, tag="outt")
        for c0, c1 in MM_CHUNKS:
            ps = post_ps.tile([COUT, 512], FP32, tag="ps")
            nc.tensor.matmul(ps[:, :c1 - c0], cv('finT'), yc[:, c0:c1], start=True, stop=True)
            nc.scalar.activation(outt[:, c0:c1], ps[:, :c1 - c0], F.Identity, bias=cv('finb'))
        nc.sync.dma_start(out_d[:], outt[:])
        po.close()
        glob.close()
    split_multi_waits(nc, max_waits=1)
    return nc


# =============================== host side ==================================

def prep_core_inputs(inputs, b, half):
    import ml_dtypes
    f32 = np.float32
    bf16 = ml_dtypes.bfloat16
    d0 = half * HH

    def asf(a):
        return np.asarray(a, f32)

    bnscale = asf(inputs['bn_g']) / np.sqrt(np.float32(1.0 + 1e-5))
    w1 = asf(inputs['conv1_w'])[:, :, 0, 0] * bnscale[:, None]
    b1 = asf(inputs['conv1_b']) * bnscale + asf(inputs['bn_b'])

    def diag9(w, nch):
        out = np.zeros((nch, 9 * nch), f32)
        w = asf(w)
        for tap in range(9):
            dy, dx = tap // 3, tap % 3
            blk = out[:, tap * nch:(tap + 1) * nch]
            np.fill_diagonal(blk, w[:, 0, dy, dx])
        return out

    sscd = diag9(inputs['ss_conv_w'], DI)        # (192, 9*192)
    sc0 = np.zeros((128, 9 * 128), f32)
    sc1 = np.zeros((64, 9 * 64), f32)
    for tap in range(9):
        blk = sscd[:, tap * DI:(tap + 1) * DI]
        sc0[:, tap * 128:(tap + 1) * 128] = blk[0:128, 0:128]
        sc1[:, tap * 64:(tap + 1) * 64] = blk[128:192, 128:192]

    sel = np.zeros((DI, HH), f32)
    sel[np.arange(d0, d0 + HH), np.arange(HH)] = 1.0

    xp = asf(inputs['ss_xproj_w'])               # (4, 38, 192)
    xpTa = np.zeros((128, K4 * 38), f32)
    xpTb = np.zeros((64, K4 * 38), f32)
    for k in range(K4):
        xpT = xp[k].T                            # (192, 38)
        xpTa[:, k * 38:(k + 1) * 38] = xpT[0:128]
        xpTb[:, k * 38:(k + 1) * 38] = xpT[128:192]

    dtw = asf(inputs['ss_dt_w'])
    dtwT = np.zeros((R, K4 * HH), f32)
    for k in range(K4):
        dtwT[:, k * HH:(k + 1) * HH] = dtw[k][d0:d0 + HH, :].T

    dtb_full = asf(inputs['ss_dt_b'])
    Alog = asf(inputs['ss_Alog']).reshape(K4, DI, N)
    Dv = asf(inputs['ss_D']).reshape(K4, DI)
    dtb_p = np.zeros((128, NT), f32)
    Ap = np.zeros((128, NT * N), f32)
    for (j, o0, o1, k, dd0, dd1) in SECTIONS:
        dtb_p[o0:o1, j] = dtb_full[k, d0 + dd0:d0 + dd1]
        Ap[o0:o1, j * N:(j + 1) * N] = -np.exp(Alog[k, d0 + dd0:d0 + dd1])
    Dsum = Dv[:, d0:d0 + HH].sum(0)[:, None]

    gw = asf(inputs['gwc_w'])
    gT = np.zeros((24, 9 * CH), f32)
    for tap in range(9):
        dy, dx = tap // 3, tap % 3
        blk = np.zeros((24, CH), f32)
        blk[0:12, 0:48] = gw[0:48, :, dy, dx].T
        blk[12:24, 48:96] = gw[48:96, :, dy, dx].T
        gT[:, tap * CH:(tap + 1) * CH] = blk

    owT = asf(inputs['ss_out_w']).T              # (192, 96)
    outn_g = asf(inputs['ss_outn_g'])
    outn_b = asf(inputs['ss_outn_b'])

    vals32 = {
        'b1': b1[:, None],
        'linb': asf(inputs['lin_b'])[:, None],
        'dw1b': asf(inputs['dw1_b'])[:, None],
        'dw2b': asf(inputs['dw2_b'])[:, None],
        'scb0': asf(inputs['ss_conv_b'])[0:128, None],
        'scb1': asf(inputs['ss_conv_b'])[128:192, None],
        'dtb': dtb_p, 'Ap': Ap, 'Dsum': Dsum,
        'outngA': outn_g[0:96, None], 'outngB': outn_g[96:192, None],
        'outnbA': outn_b[0:96, None], 'outnbB': outn_b[96:192, None],
        'ag1b': asf(inputs['ag1_b'])[:, None],
        'ag2b': asf(inputs['ag2_b'])[:, None],
        'lng': asf(inputs['ln_g'])[:, None],
        'lnb': asf(inputs['ln_b'])[:, None],
        'gwcb': asf(inputs['gwc_b'])[:, None],
        'finb': asf(inputs['fin_b'])[:, None],
        'bcv': np.concatenate([np.full((1, 96), 1.0 / DI, f32),
                               np.full((1, 96), 1.0 / CH, f32),
                               np.ones((1, 96), f32)], axis=1),
    }
    valsbf = {
        'w1T': w1.T,
        'linT': asf(inputs['lin_w']).T,
        'dw1dg': diag9(inputs['dw1_w'], CH),
        'dw2dg': diag9(inputs['dw2_w'], CH),
        'inwT': asf(inputs['ss_in_w']).T,        # (96, 384) full z
        'sc0dg': sc0, 'sc1dg': sc1,
        'sel0': sel[0:128], 'sel1': sel[128:192],
        'xpTa': xpTa, 'xpTb': xpTb,
        'dtwT': dtwT,
        'ident': np.eye(128, dtype=f32),
        'outwTa': owT[0:96], 'outwTb': owT[96:192],
        'ag1T': asf(inputs['ag1_w'])[:, :, 0, 0].T,
        'ag2T': asf(inputs['ag2_w'])[:, :, 0, 0].T,
        'sq1T': asf(inputs['sq1_w'])[:, :, 0, 0].T,
        'sq2T': asf(inputs['sq2_w'])[:, :, 0, 0].T,
        'gwcT': gT,
        'pw1T': asf(inputs['pwc1_w'])[:, :, 0, 0].T,
        'pw2T': asf(inputs['pwc2_w'])[:, :, 0, 0].T,
        'finT': asf(inputs['fin_w']).T,
    }

    blob32 = np.zeros((128, W32), f32)
    for nm, p, c in CONSTS_F32:
        o = OFF32[nm][0]
        v = vals32[nm]
        assert v.shape == (p, c), (nm, v.shape, (p, c))
        blob32[0:p, o:o + c] = v
    blobbf = np.zeros((128, WBF), bf16)
    for nm, p, c in CONSTS_BF16:
        o = OFFBF[nm][0]
        v = valsbf[nm]
        assert v.shape == (p, c), (nm, v.shape, (p, c))
        blobbf[0:p, o:o + c] = v.astype(bf16)

    return {
        'x': np.ascontiguousarray(asf(inputs['x'])[b].reshape(CIN, L).astype(bf16)),
        'c32': blob32,
        'cbf': np.ascontiguousarray(blobbf),
    }


_NC_CACHE = {}


def get_nc():
    if 'nc' not in _NC_CACHE:
        _NC_CACHE['nc'] = build_nc()
    return _NC_CACHE['nc']


def kernel(**inputs):
    from concourse.bass_utils import run_bass_kernel_spmd
    nc = get_nc()
    in_maps = [prep_core_inputs(inputs, c // 2, c % 2) for c in range(8)]
    res = run_bass_kernel_spmd(nc, in_maps, core_ids=list(range(8)))
    out = np.zeros((B_, COUT, H, W), np.float32)
    for b in range(B_):
        out[b] = res.results[2 * b]['out'].reshape(COUT, H, W)
    return out


# revision 60
# speedup vs baseline: 1.0612x; 1.0275x over previous
"""Self-contained Trainium2 Bass kernel for the CR-VSS block (8 cores)."""

# ---- TileContext drain-wait patch (walrus 1-wait limit) ----
"""Patch TileContext._drain_and_barrier: the axon-client walrus rejects
instructions carrying >2 sem waits ("Too many sync wait commands" in
setupSyncWait for CTRL structs). Redistribute the exit-drain's waits across
preceding SP nop instructions, each carrying at most MAX_WAITS."""
from concourse.tile import TileContext, ScopedClock

MAX_WAITS = 1


def _patched_drain_and_barrier(self, tick_clock, wait_clock):
    nc = self.nc
    drain_inst = nc.sync.drain()
    wait_clock.add_sem_waits(
        drain_inst.ins, ScopedClock({None: tick_clock.global_clock})
    )

    waits = list(drain_inst.ins.sync_info.on_wait or [])
    if len(waits) > MAX_WAITS:
        bb = nc.cur_bb.bb
        assert bb.instructions[-1] is drain_inst.ins
        # strip waits from the drain, re-emit them on nop carriers
        drain_inst.ins.sync_info.on_wait = waits[:0]
        carriers = []
        import concourse.mybir as mybir
        for i in range(0, len(waits), MAX_WAITS):
            nop = nc.sync.nop(nofuse=True)
            nop.ins.sync_info = mybir.SyncInfo(
                on_wait=waits[i:i + MAX_WAITS], on_update=[]
            )
            carriers.append(nop.ins)
        # move carriers before the drain
        insts = list(bb.instructions)
        assert insts[-len(carriers) - 1] is drain_inst.ins
        reordered = insts[:-len(carriers) - 1] + insts[-len(carriers):] + [drain_inst.ins]
        while len(bb.instructions):
            bb.instructions.pop()
        for x in reordered:
            bb.instructions.append(x)

    nc.all_engine_barrier()
    assert self.sems is not None
    popped = nc._tile_sem_poison_stack.pop()
    assert popped is self._sem_poison
    nc.clear_and_free_semaphores(list(self.sems.allocated().values()))
    nc.all_engine_barrier()


def apply():
    TileContext._drain_and_barrier = _patched_drain_and_barrier


def split_multi_waits(nc, max_waits=1):
    """Post-pass: walrus CTRL codegen rejects instructions with more than
    one sem wait. Move extra waits onto same-engine NoOp carriers."""
    import concourse.mybir as mybir
    for f in nc.m.functions:
        for bb in f.blocks:
            insts = list(bb.instructions)
            out = []
            changed = False
            for ins in insts:
                si = ins.sync_info
                if si is not None and si.on_wait and len(si.on_wait) > max_waits:
                    waits = list(si.on_wait)
                    for i, w in enumerate(waits[max_waits:]):
                        nop = mybir.InstNoOp.__new__(
                            mybir.InstNoOp, name=f"{ins.name}-xw{i}", ins=[], outs=[])
                        nop.engine = ins.engine
                        nop.sync_info = mybir.SyncInfo(on_wait=[w], on_update=[])
                        out.append(nop)
                    ins.sync_info = mybir.SyncInfo(
                        on_wait=waits[:max_waits],
                        on_update=list(si.on_update or []))
                    changed = True
                out.append(ins)
            if changed:
                while len(bb.instructions):
                    bb.instructions.pop()
                for x in out:
                    bb.instructions.append(x)

apply()

# ---- kernel ----
"""Trainium2 Bass kernel for nn_CR_VSS (VSS block with SS2D selective scan).

Sharding: 8 cores = 4 samples x 2 d_inner-halves. Each core runs the full
pre-stage for its sample, scans its 96-channel d-half across all 4
cross-scan directions (packed into 3x128-partition tiles), then the pair
exchanges y-halves with ONE AllGather; LN + out-proj + post-stage run
locally (z is computed full-width in the in-proj so no second collective).

Scan: h_t = exp(A*delta_t)*h_{t-1} + delta_t*u_t*B_t per (k,d,n) via
tensor_tensor_scan; n in groups of 4 with batched B/C partition-broadcast
DMAs (double-buffered); y accumulated over n with identity-lhsT PSUM
matmuls, merged into pixel-order ysum straight from PSUM per t-chunk.
"""
import numpy as np
from contextlib import ExitStack

import concourse.bass as bass
import concourse.mybir as mybir

F = mybir.ActivationFunctionType
A = mybir.AluOpType
FP32 = mybir.dt.float32
BF16 = mybir.dt.bfloat16

B_, CIN, CH, COUT, H, W = 4, 96, 96, 96, 48, 48
DI, N, R, K4 = 192, 16, 6, 4
L = H * W               # 2304
HH = 96                 # d-half per core
NT = 3                  # packed (k,d) tiles: 4*96 = 384 = 3*128
HP = 50
LP = 2500
TC = 768                # scan t-chunk (16 rows of 48)
TCH = [(0, 768), (768, 1536), (1536, 2304)]
NG = 4                  # scan n-group (broadcast batch)

# packed (k,d) rows -> (tile j, offset): sections (j, o0, o1, k, d0, d1).
# Section offsets are all 0/32/64 so PE matmuls can write them directly.
SECTIONS = [
    (0, 0, 32, 1, 0, 32),
    (0, 32, 128, 0, 0, 96),
    (1, 0, 64, 1, 32, 96),
    (1, 64, 128, 2, 0, 64),
    (2, 0, 32, 2, 64, 96),
    (2, 32, 128, 3, 0, 96),
]

MM_CHUNKS = [(0, 512), (512, 1024), (1024, 1536), (1536, 2048), (2048, 2304)]
ROW_CHUNKS = [(0, 10), (10, 20), (20, 30), (30, 40), (40, 48)]
SUBS768 = [(0, 512), (512, 768)]
INW_BLOCKS = [(0, 128), (128, 256), (256, 384)]

REPLICA_GROUPS = [[0, 1], [2, 3], [4, 5], [6, 7]]

# ---- const blobs (shared layout between host packing and kernel views) ----
CONSTS_F32 = [
    ('b1', 96, 1), ('linb', 96, 1),
    ('dw1b', 96, 1), ('dw2b', 96, 1),
    ('scb0', 128, 1), ('scb1', 64, 1),
    ('dtb', 128, 3), ('Ap', 128, 48), ('Dsum', 96, 1),
    ('outngA', 96, 1), ('outngB', 96, 1), ('outnbA', 96, 1), ('outnbB', 96, 1),
    ('ag1b', 48, 1), ('ag2b', 96, 1), ('lng', 96, 1), ('lnb', 96, 1),
    ('gwcb', 96, 1), ('finb', 96, 1),
]
CONSTS_BF16 = [
    ('w1T', 96, 96), ('linT', 96, 96),
    ('dw1dg', 96, 864), ('dw2dg', 96, 864),
    ('inwT', 96, 384),
    ('sc0dg', 128, 1152), ('sc1dg', 64, 576),
    ('sel0', 128, 96), ('sel1', 64, 96),
    ('xpTa', 128, 152), ('xpTb', 64, 152),
    ('dtwT', 6, 384),
    ('ident', 128, 128),
    ('outwTa', 96, 96), ('outwTb', 96, 96),
    ('ag1T', 96, 48), ('ag2T', 48, 96),
    ('sq1T', 48, 24), ('sq2T', 48, 24),
    ('gwcT', 24, 864), ('pw1T', 24, 96), ('pw2T', 24, 72),
    ('finT', 96, 96), ('bcv', 1, 288),
]

OFF32 = {}
_o = 0
for _nm, _p, _c in CONSTS_F32:
    OFF32[_nm] = (_o, _p, _c)
    _o += _c
W32 = _o
OFFBF = {}
_o = 0
for _nm, _p, _c in CONSTS_BF16:
    OFFBF[_nm] = (_o, _p, _c)
    _o += _c
WBF = _o


def build_nc():
    nc = bass.Bass(trn_type="TRN2", num_devices=8)

    x_d = nc.dram_tensor("x", [CIN, L], BF16, kind="ExternalInput")
    c32_d = nc.dram_tensor("c32", [128, W32], FP32, kind="ExternalInput")
    cbf_d = nc.dram_tensor("cbf", [128, WBF], BF16, kind="ExternalInput")
    out_d = nc.dram_tensor("out", [COUT, L], FP32, kind="ExternalOutput")

    B_dram = nc.dram_tensor("B_dram", [K4 * N, L], BF16)
    C_dram = nc.dram_tensor("C_dram", [K4 * N, L], BF16)
    y_dram = nc.dram_tensor("y_dram", [HH, L], BF16)
    yg_dram = nc.dram_tensor("yg_dram", [DI, L], BF16)
    st_dram = nc.dram_tensor("st_dram", [2, L], BF16)

    def hw(ap):
        return ap.rearrange("p (h w) -> p h w", h=H)

    def hwp(ap):
        return ap.rearrange("p (h w) -> p h w", h=HP)

    def whv(ap):
        return ap.rearrange("p (h w) -> p w h", h=H)

    with TileContext(nc) as tc:
        glob = ExitStack()
        cst = glob.enter_context(tc.tile_pool(name="cst", bufs=1))
        lngA = glob.enter_context(tc.tile_pool(name="lngA", bufs=1))

        cst32 = cst.tile([128, W32], FP32, tag="cst32")
        cstbf = cst.tile([128, WBF], BF16, tag="cstbf")
        nc.sync.dma_start(cst32[:], c32_d[:])
        nc.sync.dma_start(cstbf[:], cbf_d[:])

        def cvc(nm, a0=0, a1=None, p0=0, p1=None):
            d, tile = (OFF32, cst32) if nm in OFF32 else (OFFBF, cstbf)
            o, p, c = d[nm]
            if a1 is None:
                a1 = c
            if p1 is None:
                p1 = p
            return tile[p0:p1, o + a0:o + a1]

        cv = cvc

        ones96 = cst.tile([HH, 1], BF16, tag="ones96")
        nc.vector.memset(ones96[:], 1.0)

        # long-lived across phases
        z0 = lngA.tile([128, L], BF16, tag="z0")     # z rows 0:128
        z1 = lngA.tile([64, L], BF16, tag="z1")      # z rows 128:192
        x2 = lngA.tile([CH, L], BF16, tag="x2")
        lngB = ExitStack()
        lngB_p = lngB.enter_context(tc.tile_pool(name="lngB_p", bufs=1))
        xch = lngB_p.tile([HH, L], BF16, tag="xch")
        dp = [lngB_p.tile([128, L], BF16, tag=f"dp{j}", name=f"dp{j}") for j in range(NT)]
        # xsp holds packed scan-order xs, overwritten in place with delta*u
        xsp = [lngB_p.tile([128, L], BF16, tag=f"xsp{j}", name=f"xsp{j}") for j in range(NT)]
        ysum = lngB_p.tile([HH, L], BF16, tag="ysum")

        # ================= pre-stage =================
        pre = ExitStack()
        pre_ps = pre.enter_context(tc.tile_pool(name="pre_ps", bufs=4, space="PSUM"))
        pA = pre.enter_context(tc.tile_pool(name="pA", bufs=1))
        pB = pre.enter_context(tc.tile_pool(name="pB", bufs=1))

        xt = pA.tile([CIN, L], BF16, tag="xt")
        nc.sync.dma_start(xt[:], x_d[:])

        # conv1x1 (+folded BN) + ReLU
        h1 = pA.tile([CH, L], BF16, tag="h1")
        for c0, c1 in MM_CHUNKS:
            ps = pre_ps.tile([CH, 512], FP32, tag="ps")
            nc.tensor.matmul(ps[:, :c1 - c0], cv('w1T'), xt[:, c0:c1], start=True, stop=True)
            nc.scalar.activation(h1[:, c0:c1], ps[:, :c1 - c0], F.Relu, bias=cv('b1'))
        # token linear
        h2 = pA.tile([CH, L], BF16, tag="h2")
        for c0, c1 in MM_CHUNKS:
            ps = pre_ps.tile([CH, 512], FP32, tag="ps")
            nc.tensor.matmul(ps[:, :c1 - c0], cv('linT'), h1[:, c0:c1], start=True, stop=True)
            nc.vector.tensor_scalar(out=h2[:, c0:c1], in0=ps[:, :c1 - c0],
                                    scalar1=cv('linb'), scalar2=None, op0=A.add)
        h2p = pA.tile([CH, LP], BF16, tag="h2p")
        nc.gpsimd.memset(h2p[:], 0.0)
        for (r0, r1) in ROW_CHUNKS:
            nc.vector.tensor_copy(hwp(h2p[:])[:, r0 + 1:r1 + 1, 1:49],
                                  hw(h2[:])[:, r0:r1, :])

        def dwconv(dst, src_p, dgname, biasname, nch):
            for (r0, r1) in ROW_CHUNKS:
                nr = r1 - r0
                ps = pre_ps.tile([128, 480], FP32, tag="ps")
                for tap in range(9):
                    dy, dx = tap // 3, tap % 3
                    rhs = hwp(src_p[:])[:, dy + r0:dy + r1, dx:dx + 48]
                    nc.tensor.matmul(ps[:nch, :nr * 48],
                                     cvc(dgname, tap * nch, (tap + 1) * nch),
                                     rhs, start=(tap == 0), stop=(tap == 8))
                nc.scalar.activation(dst[:, r0 * 48:r1 * 48], ps[:nch, :nr * 48],
                                     F.Silu, bias=cv(biasname))

        x1 = pB.tile([CH, L], BF16, tag="x1")
        dwconv(x1, h2p, 'dw1dg', 'dw1b', CH)

        # in-proj: xi (192) + FULL z (192)
        xi0 = pB.tile([128, L], BF16, tag="xi0")
        xi1 = pB.tile([64, L], BF16, tag="xi1")
        for mb, (m0, m1) in enumerate(INW_BLOCKS):
            for c0, c1 in MM_CHUNKS:
                ps = pre_ps.tile([128, 512], FP32, tag="ps")
                nc.tensor.matmul(ps[:m1 - m0, :c1 - c0], cvc('inwT', m0, m1),
                                 x1[:, c0:c1], start=True, stop=True)
                if mb == 0:
                    nc.vector.tensor_copy(xi0[:, c0:c1], ps[:128, :c1 - c0])
                elif mb == 1:
                    nc.scalar.copy(xi1[:, c0:c1], ps[0:64, :c1 - c0])
                    nc.scalar.copy(z0[0:64, c0:c1], ps[64:128, :c1 - c0])
                else:
                    nc.scalar.copy(z0[64:128, c0:c1], ps[0:64, :c1 - c0])
                    nc.scalar.copy(z1[0:64, c0:c1], ps[64:128, :c1 - c0])

        xi0p = pB.tile([128, LP], BF16, tag="xi0p")
        xi1p = pB.tile([64, LP], BF16, tag="xi1p")
        nc.gpsimd.memset(xi0p[:], 0.0)
        nc.gpsimd.memset(xi1p[:], 0.0)
        for (r0, r1) in ROW_CHUNKS:
            nc.vector.tensor_copy(hwp(xi0p[:])[:, r0 + 1:r1 + 1, 1:49],
                                  hw(xi0[:])[:, r0:r1, :])
            nc.vector.tensor_copy(hwp(xi1p[:])[:, r0 + 1:r1 + 1, 1:49],
                                  hw(xi1[:])[:, r0:r1, :])
        xc0 = pB.tile([128, L], BF16, tag="xc0")
        xc1 = pB.tile([64, L], BF16, tag="xc1")
        dwconv(xc0, xi0p, 'sc0dg', 'scb0', 128)
        dwconv(xc1, xi1p, 'sc1dg', 'scb1', 64)

        # d-half extraction + wh copy
        for c0, c1 in MM_CHUNKS:
            ps = pre_ps.tile([HH, 512], FP32, tag="ps")
            nc.tensor.matmul(ps[:, :c1 - c0], cv('sel0'), xc0[:, c0:c1], start=True, stop=False)
            nc.tensor.matmul(ps[:, :c1 - c0], cv('sel1'), xc1[:, c0:c1], start=False, stop=True)
            nc.vector.tensor_copy(xch[:, c0:c1], ps[:, :c1 - c0])
        xwhh = pB.tile([HH, L], BF16, tag="xwhh")
        for (t0, t1) in TCH:
            w0, w1 = t0 // 48, t1 // 48
            nc.vector.tensor_copy(hw(xwhh[:])[:, w0:w1, :],
                                  whv(xch[:])[:, w0:w1, :])

        # xproj (compact 38 rows: 0:6 dts, 6:22 B, 22:38 C) in scan order
        def xc_read(k, c0, c1):
            if k == 0:
                return (xc0[:, c0:c1], xc1[:, c0:c1])
            if k == 1:
                return (whv(xc0[:])[:, c0 // 48:c1 // 48, :],
                        whv(xc1[:])[:, c0 // 48:c1 // 48, :])
            if k == 2:
                return (xc0[:, L - c1:L - c0][:, ::-1],
                        xc1[:, L - c1:L - c0][:, ::-1])
            r0 = whv(xc0[:])[:, (L - c1) // 48:(L - c0) // 48, :][:, ::-1, ::-1]
            r1 = whv(xc1[:])[:, (L - c1) // 48:(L - c0) // 48, :][:, ::-1, ::-1]
            return (r0, r1)

        # row-chunk outer so all 4 directions' early columns finish first;
        # B/C are written to DRAM per scan chunk so ci=0 broadcasts can
        # start while xproj still works on later chunks.
        stage = [pB.tile([38, L], BF16, tag=f"stg{k}", name=f"stg{k}") for k in range(K4)]
        done_w = 0
        for ri, (rr0, rr1) in enumerate(ROW_CHUNKS):
            c0, c1 = rr0 * 48, rr1 * 48
            nf = c1 - c0
            for k in range(K4):
                ra, rb = xc_read(k, c0, c1)
                ps = pre_ps.tile([38, 480], FP32, tag="ps")
                nc.tensor.matmul(ps[:, :nf], cvc('xpTa', k * 38, (k + 1) * 38), ra,
                                 start=True, stop=False)
                nc.tensor.matmul(ps[:, :nf], cvc('xpTb', k * 38, (k + 1) * 38), rb,
                                 start=False, stop=True)
                nc.vector.tensor_copy(stage[k][:, c0:c1], ps[:, :nf])
            while done_w < len(TCH) and TCH[done_w][1] <= c1:
                t0, t1 = TCH[done_w]
                for k in range(K4):
                    nc.sync.dma_start(B_dram[k * N:(k + 1) * N, t0:t1],
                                      stage[k][6:22, t0:t1])
                    nc.sync.dma_start(C_dram[k * N:(k + 1) * N, t0:t1],
                                      stage[k][22:38, t0:t1])
                done_w += 1

        # delta: packed matmuls then softplus on full 128-partition tiles
        def mm_windows(a0, a1):
            if a0 == 0:
                return [(0, a1)]
            res = []
            x = a0
            while x < a1:
                if x % 64 == 32:
                    e = min(a1, x + 32)
                else:  # x == 64
                    e = min(a1, 128)
                res.append((x, e))
                x = e
            return res

        for (cc0, cc1) in MM_CHUNKS:
            cw = cc1 - cc0
            for j in range(NT):
                ex = pre_ps.tile([128, 512], FP32, tag="ps")
                for (jj, o0, o1, k, d0, d1) in SECTIONS:
                    if jj != j:
                        continue
                    for (w0, w1) in mm_windows(o0, o1):
                        dd0 = d0 + (w0 - o0)
                        dd1 = d0 + (w1 - o0)
                        nc.tensor.matmul(ex[w0:w1, :cw],
                                         cvc('dtwT', k * 96 + dd0, k * 96 + dd1),
                                         stage[k][0:6, cc0:cc1], start=True, stop=True)
                # softplus(x+b) = ln(1 + exp(x+b)) (no softplus act table on HW)
                ex2 = pre_ps.tile([128, 512], FP32, tag="ps")
                nc.scalar.activation(ex2[:, :cw], ex[:, :cw], F.Exp,
                                     bias=cvc('dtb', j, j + 1))
                nc.scalar.activation(dp[j][:, cc0:cc1], ex2[:, :cw], F.Ln, bias=1.0)

        # pack scan-order xs (Act copies handle partition shift + flips),
        # then overwrite in place with delta*u = dp*xs.
        # Act partition windows must not cross engine block boundaries on
        # EITHER side: allowed starts 0/32/64/96; a start-32 window may not
        # cross 64. split2 chops a shifted copy accordingly.
        def _legal_span(s):
            return 32 if s == 32 else 128 - s if s else 128

        def split2(o0, i0, ln):
            res = []
            x = 0
            while x < ln:
                step = min(ln - x, _legal_span(o0 + x), _legal_span(i0 + x))
                res.append((x, x + step))
                x += step
            return res

        for (t0, t1) in TCH:
            for (j, o0, o1, k, d0, d1) in SECTIONS:
                v = xwhh if k in (1, 3) else xch
                if k < 2:
                    # forward sections: contiguous rows, cheap DMA shift
                    nc.sync.dma_start(xsp[j][o0:o1, t0:t1], v[d0:d1, t0:t1])
                    continue
                for (w0, w1) in split2(o0, d0, o1 - o0):
                    nc.scalar.copy(xsp[j][o0 + w0:o0 + w1, t0:t1],
                                   v[d0 + w0:d0 + w1, ::-1][:, t0:t1])
            for j in range(NT):
                nc.vector.tensor_mul(xsp[j][:, t0:t1], dp[j][:, t0:t1],
                                     xsp[j][:, t0:t1])

        # branch-2 dwconv last: x2 is only needed in the post stage, so
        # this fills PE/Act slack once the scan inputs are queued
        dwconv(x2, h2p, 'dw2dg', 'dw2b', CH)

        pre.close()

        # ================= scan =================
        sc = ExitStack()
        bbp = sc.enter_context(tc.tile_pool(name="bbp", bufs=2))
        spool = sc.enter_context(tc.tile_pool(name="spool", bufs=2))
        scan_ps = sc.enter_context(tc.tile_pool(name="scan_ps", bufs=1, space="PSUM"))
        stp = sc.enter_context(tc.tile_pool(name="stp", bufs=1))
        state = [stp.tile([128, N], FP32, tag=f"st{j}", name=f"state{j}")
                 for j in range(NT)]
        yd = [stp.tile([HH, L], BF16, tag=f"yd{k}", name=f"yd{k}") for k in range(K4)]

        pending_drain = None
        for ci, (c0, c1) in enumerate(TCH):
            ypsum = [scan_ps.tile([128, TC], FP32, tag=f"yps{j}", name=f"yps{j}_{ci}")
                     for j in range(NT)]
            for g in range(K4):
                Bb = [bbp.tile([128, NG * TC], BF16, tag=f"Bb{j}", name=f"Bb{j}_{ci}_{g}")
                      for j in range(NT)]
                Cb = [bbp.tile([128, NG * TC], BF16, tag=f"Cb{j}", name=f"Cb{j}_{ci}_{g}")
                      for j in range(NT)]
                for (j, o0, o1, k, d0, d1) in SECTIONS:
                    nc.sync.dma_start(
                        Bb[j][o0:o1, :],
                        B_dram[k * N + NG * g:k * N + NG * g + NG, c0:c1]
                        .partition_broadcast(o1 - o0))
                    nc.scalar.dma_start(
                        Cb[j][o0:o1, :],
                        C_dram[k * N + NG * g:k * N + NG * g + NG, c0:c1]
                        .partition_broadcast(o1 - o0))
                if pending_drain is not None:
                    pending_drain()
                    pending_drain = None
                for n4 in range(NG):
                    n = NG * g + n4
                    for j in range(NT):
                        at = spool.tile([128, TC], BF16, tag=f"at{j}", name=f"at{j}")
                        nc.scalar.activation(at[:], dp[j][:, c0:c1], F.Exp,
                                             scale=cvc('Ap', j * N + n, j * N + n + 1))
                        bt = spool.tile([128, TC], BF16, tag=f"bt{j}", name=f"bt{j}")
                        nc.vector.tensor_mul(bt[:], xsp[j][:, c0:c1],
                                             Bb[j][:, n4 * TC:(n4 + 1) * TC])
                        ht = spool.tile([128, TC], BF16, tag=f"ht{j}", name=f"ht{j}")
                        if ci > 0:
                            # fold carry state into bt[0] so the scan can use
                            # the cheap zero-init form
                            nc.vector.scalar_tensor_tensor(
                                out=bt[:, 0:1], in0=at[:, 0:1],
                                scalar=state[j][:, n:n + 1], in1=bt[:, 0:1],
                                op0=A.mult, op1=A.add)
                        nc.vector.tensor_tensor_scan(ht[:], at[:], bt[:], 0.0,
                                                     A.mult, A.add)
                        if ci < 2:
                            nc.vector.tensor_copy(state[j][:, n:n + 1], ht[:, TC - 1:TC])
                        gt = spool.tile([128, TC], BF16, tag=f"gt{j}", name=f"gt{j}")
                        nc.vector.tensor_mul(gt[:], ht[:],
                                             Cb[j][:, n4 * TC:(n4 + 1) * TC])
                        for (s0, s1) in SUBS768:
                            nc.tensor.matmul(ypsum[j][:, s0:s1], cv('ident'),
                                             gt[:, s0:s1],
                                             start=(n == 0), stop=(n == N - 1))
            # drain this chunk's PSUM into per-direction scan-order tiles
            # (Act copies allow the partition shift). Deferred past the next
            # chunk's broadcast issue so the boundary doesn't stall Act.
            def _drain(yps=ypsum, cc0=c0, cc1=c1):
                for (j, o0, o1, k, d0, d1) in SECTIONS:
                    for (w0, w1) in split2(d0, o0, d1 - d0):
                        nc.scalar.copy(yd[k][d0 + w0:d0 + w1, cc0:cc1],
                                       yps[j][o0 + w0:o0 + w1, :])
            pending_drain = _drain
        if pending_drain is not None:
            pending_drain()
            pending_drain = None
        # merge directions into pixel order + D*u term
        tmp96 = stp.tile([HH, L], BF16, tag="tmp96")
        nc.vector.tensor_add(ysum[:], yd[0][:], yd[2][:, ::-1])
        nc.vector.tensor_add(tmp96[:], yd[1][:], yd[3][:, ::-1])
        nc.vector.tensor_add(ysum[:], ysum[:], whv(tmp96[:]))
        nc.vector.scalar_tensor_tensor(out=ysum[:], in0=xch[:], scalar=cv('Dsum'),
                                       in1=ysum[:], op0=A.mult, op1=A.add)
        nc.sync.dma_start(y_dram[:], ysum[:])
        nc.gpsimd.collective_compute(
            "AllGather", A.bypass, replica_groups=REPLICA_GROUPS,
            ins=[y_dram[:]], outs=[yg_dram[:]])
        sc.close()
        lngB.close()

        # ================= post =================
        po = ExitStack()
        post_ps = po.enter_context(tc.tile_pool(name="post_ps", bufs=4, space="PSUM"))
        pP = po.enter_context(tc.tile_pool(name="pP", bufs=1))
        rot = po.enter_context(tc.tile_pool(name="rot", bufs=4))

        # branch 2 + silu(z): no dep on the collective, runs under it
        g1 = rot.tile([48, L], BF16, tag="pb")
        for c0, c1 in MM_CHUNKS:
            ps = post_ps.tile([48, 512], FP32, tag="ps")
            nc.tensor.matmul(ps[:, :c1 - c0], cv('ag1T'), x2[:, c0:c1], start=True, stop=True)
            nc.scalar.activation(g1[:, c0:c1], ps[:, :c1 - c0], F.Relu, bias=cv('ag1b'))
        gat = rot.tile([CH, L], BF16, tag="pb")
        for c0, c1 in MM_CHUNKS:
            ps = post_ps.tile([CH, 512], FP32, tag="ps")
            nc.tensor.matmul(ps[:, :c1 - c0], cv('ag2T'), g1[:, c0:c1], start=True, stop=True)
            nc.scalar.activation(gat[:, c0:c1], ps[:, :c1 - c0], F.Sigmoid, bias=cv('ag2b'))
        x2g = pP.tile([CH, L], BF16, tag="x2g")
        nc.vector.tensor_mul(x2g[:], x2[:], gat[:])

        zsA = pP.tile([HH, L], BF16, tag="zsA")
        zsB = pP.tile([HH, L], BF16, tag="zsB")
        nc.scalar.activation(zsA[:], z0[0:96, :], F.Silu)
        nc.scalar.activation(zsB[0:32, :], z0[96:128, :], F.Silu)
        nc.scalar.activation(zsB[32:64, :], z1[0:32, :], F.Silu)
        nc.scalar.activation(zsB[64:96, :], z1[32:64, :], F.Silu)

        ygA = pP.tile([HH, L], BF16, tag="ygA")
        ygB = pP.tile([HH, L], BF16, tag="ygB")
        nc.sync.dma_start(ygA[:], yg_dram[0:HH, :])
        nc.sync.dma_start(ygB[:], yg_dram[HH:DI, :])

        sA = pP.tile([1, L], BF16, tag="sA")
        sB = pP.tile([1, L], BF16, tag="sB")
        sM = pP.tile([1, L], BF16, tag="sM")

        def ln_stats(cinv):
            # in: sA=raw sum, sB=raw sumsq; leaves rstd in sB (sA stays raw sum)
            with nc.allow_low_precision(reason="LN stats kept bf16; rel-err verified"):
                nc.scalar.activation(sM[:], sA[:], F.Square, scale=cinv)
                nc.vector.scalar_tensor_tensor(out=sB[:], in0=sB[:], scalar=cinv,
                                               in1=sM[:], op0=A.mult, op1=A.subtract)
                nc.vector.tensor_scalar(out=sB[:], in0=sB[:], scalar1=1e-5,
                                        scalar2=None, op0=A.add)
                nc.vector.reciprocal(sB[:], sB[:])
                nc.scalar.activation(sB[:], sB[:], F.Sqrt)

        # LayerNorm over full DI (local stats via ones-matmul)
        ysqA = rot.tile([HH, L], BF16, tag="pb")
        ysqB = rot.tile([HH, L], BF16, tag="pb")
        nc.vector.tensor_mul(ysqA[:], ygA[:], ygA[:])
        nc.vector.tensor_mul(ysqB[:], ygB[:], ygB[:])
        for c, (c0, c1) in enumerate(MM_CHUNKS):
            ps = post_ps.tile([1, 512], FP32, tag="ps")
            nc.tensor.matmul(ps[:, :c1 - c0], ones96[:], ygA[:, c0:c1], start=True, stop=False)
            nc.tensor.matmul(ps[:, :c1 - c0], ones96[:], ygB[:, c0:c1], start=False, stop=True)
            nc.scalar.copy(sA[0:1, c0:c1], ps[:, :c1 - c0])
            ps2 = post_ps.tile([1, 512], FP32, tag="ps")
            nc.tensor.matmul(ps2[:, :c1 - c0], ones96[:], ysqA[:, c0:c1], start=True, stop=False)
            nc.tensor.matmul(ps2[:, :c1 - c0], ones96[:], ysqB[:, c0:c1], start=False, stop=True)
            nc.scalar.copy(sB[0:1, c0:c1], ps2[:, :c1 - c0])

        ln_stats(1.0 / DI)

        def apply_ln(pairs, bco):
            # pairs: list of (dst, src, gname, bname); bco: bcv column offset
            # holding 1/DI or 1/CH (folds the mean division into the
            # broadcast lhsT)
            for c0, c1 in MM_CHUNKS:
                cw = c1 - c0
                psm = post_ps.tile([HH, 512], FP32, tag="ps")
                nc.tensor.matmul(psm[:, :cw], cvc('bcv', bco, bco + HH),
                                 sA[:, c0:c1], start=True, stop=True)
                psr = post_ps.tile([HH, 512], FP32, tag="ps")
                nc.tensor.matmul(psr[:, :cw], cvc('bcv', 192, 192 + HH),
                                 sB[:, c0:c1], start=True, stop=True)
                for (dst, srct, gname, bname) in pairs:
                    nc.vector.tensor_sub(dst[:, c0:c1], srct[:, c0:c1],
                                         psm[:, :cw])
                    nc.vector.tensor_mul(dst[:, c0:c1], dst[:, c0:c1],
                                         psr[:, :cw])
                    nc.vector.tensor_scalar(out=dst[:, c0:c1], in0=dst[:, c0:c1],
                                            scalar1=cv(gname), scalar2=cv(bname),
                                            op0=A.mult, op1=A.add)

        ynA = rot.tile([HH, L], BF16, tag="pb")
        ynB = rot.tile([HH, L], BF16, tag="pb")
        apply_ln([(ynA, ygA, 'outngA', 'outnbA'),
                  (ynB, ygB, 'outngB', 'outnbB')], 0)

        # fused chunk pipeline: ygz -> out-proj -> yb -> ybsq -> LN2 stats
        gzA = rot.tile([HH, L], BF16, tag="pb")
        gzB = rot.tile([HH, L], BF16, tag="pb")
        x1o = pP.tile([CH, L], BF16, tag="x1o")
        yb = pP.tile([CH, L], BF16, tag="yb")
        ybsq = rot.tile([CH, L], BF16, tag="pb")
        for c0, c1 in MM_CHUNKS:
            cw = c1 - c0
            nc.vector.tensor_mul(gzA[:, c0:c1], ynA[:, c0:c1], zsA[:, c0:c1])
            nc.vector.tensor_mul(gzB[:, c0:c1], ynB[:, c0:c1], zsB[:, c0:c1])
            ps = post_ps.tile([CH, 512], FP32, tag="ps")
            nc.tensor.matmul(ps[:, :cw], cv('outwTa'), gzA[:, c0:c1],
                             start=True, stop=False)
            nc.tensor.matmul(ps[:, :cw], cv('outwTb'), gzB[:, c0:c1],
                             start=False, stop=True)
            nc.scalar.copy(x1o[:, c0:c1], ps[:, :cw])
            nc.vector.tensor_add(yb[:, c0:c1], x1o[:, c0:c1], x2g[:, c0:c1])
            nc.vector.tensor_mul(ybsq[:, c0:c1], yb[:, c0:c1], yb[:, c0:c1])
            ps1 = post_ps.tile([1, 512], FP32, tag="ps")
            nc.tensor.matmul(ps1[:, :cw], ones96[:], yb[:, c0:c1], start=True, stop=True)
            nc.scalar.copy(sA[0:1, c0:c1], ps1[:, :cw])
            ps2 = post_ps.tile([1, 512], FP32, tag="ps")
            nc.tensor.matmul(ps2[:, :cw], ones96[:], ybsq[:, c0:c1], start=True, stop=True)
            nc.scalar.copy(sB[0:1, c0:c1], ps2[:, :cw])
        ln_stats(1.0 / CH)
        ybn = pP.tile([CH, L], BF16, tag="ybn")
        apply_ln([(ybn, yb, 'lng', 'lnb')], 96)

        # CRM
        low_t = rot.tile([48, L], BF16, tag="pb")
        for c0, c1 in MM_CHUNKS:
            nc.sync.dma_start(low_t[:, c0:c1], ybn[48:96, c0:c1])
        upc = pP.tile([24, L], BF16, tag="upc")
        lowc = pP.tile([24, L], BF16, tag="lowc")
        m2cb = pP.tile([24, 5], FP32, tag="m2cb")
        _li = {c0: i for i, (c0, c1) in enumerate(MM_CHUNKS)}.get
        for c0, c1 in MM_CHUNKS:
            ps = post_ps.tile([24, 512], FP32, tag="ps")
            nc.tensor.matmul(ps[:, :c1 - c0], cv('sq1T'), ybn[0:48, c0:c1], start=True, stop=True)
            nc.scalar.copy(upc[:, c0:c1], ps[:, :c1 - c0])
            ps2 = post_ps.tile([24, 512], FP32, tag="ps")
            nc.tensor.matmul(ps2[:, :c1 - c0], cv('sq2T'), low_t[:, c0:c1], start=True, stop=True)
            nc.scalar.activation(lowc[:, c0:c1], ps2[:, :c1 - c0], F.Identity,
                                 accum_out=m2cb[:, _li(c0):_li(c0) + 1])
        upcp = pP.tile([24, LP], BF16, tag="upcp")
        nc.gpsimd.memset(upcp[:], 0.0)
        nc.vector.tensor_copy(hwp(upcp[:])[:, 1:49, 1:49], hw(upc[:]))
        Y1 = pP.tile([CH, L], BF16, tag="Y1")
        m1c = pP.tile([CH, 5], FP32, tag="m1c")
        for ri, (r0, r1) in enumerate(ROW_CHUNKS):
            nr = r1 - r0
            ps = post_ps.tile([CH, 480], FP32, tag="ps")
            for tap in range(9):
                dy, dx = tap // 3, tap % 3
                rhs = hwp(upcp[:])[:, dy + r0:dy + r1, dx:dx + 48]
                nc.tensor.matmul(ps[:, :nr * 48], cvc('gwcT', tap * CH, (tap + 1) * CH),
                                 rhs, start=(tap == 0), stop=False)
            nc.tensor.matmul(ps[:, :nr * 48], cv('pw1T'), upc[:, r0 * 48:r1 * 48],
                             start=False, stop=True)
            nc.scalar.activation(Y1[:, r0 * 48:r1 * 48], ps[:, :nr * 48],
                                 F.Identity, bias=cv('gwcb'),
                                 accum_out=m1c[:, ri:ri + 1])
        Y2a = pP.tile([72, L], BF16, tag="Y2a")
        m2ca = pP.tile([72, 5], FP32, tag="m2ca")
        for ri, (c0, c1) in enumerate(MM_CHUNKS):
            ps = post_ps.tile([72, 512], FP32, tag="ps")
            nc.tensor.matmul(ps[:, :c1 - c0], cv('pw2T'), lowc[:, c0:c1], start=True, stop=True)
            nc.scalar.activation(Y2a[:, c0:c1], ps[:, :c1 - c0], F.Identity,
                                 accum_out=m2ca[:, ri:ri + 1])
        m1 = pP.tile([CH, 1], FP32, tag="m1")
        m2a_s = pP.tile([72, 1], FP32, tag="m2a_s")
        m2b_s = pP.tile([24, 1], FP32, tag="m2b_s")
        nc.vector.reduce_sum(m1[:], m1c[:], axis=mybir.AxisListType.X)
        nc.vector.reduce_sum(m2a_s[:], m2ca[:], axis=mybir.AxisListType.X)
        nc.vector.reduce_sum(m2b_s[:], m2cb[:], axis=mybir.AxisListType.X)
        smf = pP.tile([1, 2 * CH], FP32, tag="smf")
        nc.sync.dma_start(smf[0:1, 0:CH], m1[:, 0:1])
        nc.sync.dma_start(smf[0:1, CH:CH + 72], m2a_s[:, 0:1])
        nc.sync.dma_start(smf[0:1, CH + 72:2 * CH], m2b_s[:, 0:1])
        nc.vector.tensor_scalar(out=smf[:], in0=smf[:], scalar1=1.0 / L,
                                scalar2=None, op0=A.mult)
        mx = pP.tile([1, 1], FP32, tag="mx")
        nc.vector.reduce_max(mx[:], smf[:], axis=mybir.AxisListType.X)
        nc.vector.tensor_scalar(out=mx[:], in0=mx[:], scalar1=-1.0,
                                scalar2=None, op0=A.mult)
        nc.scalar.activation(smf[:], smf[:], F.Exp, bias=mx[0:1, 0:1])
        sm_s = pP.tile([1, 1], FP32, tag="sm_s")
        nc.vector.reduce_sum(sm_s[:], smf[:], axis=mybir.AxisListType.X)
        nc.vector.reciprocal(sm_s[:], sm_s[:])
        nc.vector.tensor_scalar(out=smf[:], in0=smf[:], scalar1=sm_s[0:1, 0:1],
                                scalar2=None, op0=A.mult)
        sm1 = pP.tile([CH, 1], FP32, tag="sm1")
        sm2 = pP.tile([CH, 1], FP32, tag="sm2")
        nc.sync.dma_start(sm1[:, 0:1], smf[0:1, 0:CH])
        nc.sync.dma_start(sm2[:, 0:1], smf[0:1, CH:2 * CH])
        o2f = rot.tile([CH, L], BF16, tag="pb")
        nc.sync.dma_start(o2f[0:72, :], Y2a[:])
        nc.sync.dma_start(o2f[72:96, :], lowc[:])
        o2t = pP.tile([CH, L], BF16, tag="o2t")
        yc = pP.tile([CH, L], BF16, tag="yc")
        outt = pP.tile([COUT, L], FP32, tag="outt")
        for c0, c1 in MM_CHUNKS:
            nc.vector.tensor_scalar(out=o2t[:, c0:c1], in0=o2f[:, c0:c1],
                                    scalar1=sm2[:, 0:1], scalar2=None, op0=A.mult)
            nc.vector.scalar_tensor_tensor(out=yc[:, c0:c1], in0=Y1[:, c0:c1],
                                           scalar=sm1[:, 0:1], in1=o2t[:, c0:c1],
                                           op0=A.mult, op1=A.add)
            ps = post_ps.tile([COUT, 512], FP32, tag="ps")
            nc.tensor.matmul(ps[:, :c1 - c0], cv('finT'), yc[:, c0:c1], start=True, stop=True)
            nc.scalar.activation(outt[:, c0:c1], ps[:, :c1 - c0], F.Identity, bias=cv('finb'))
            nc.sync.dma_start(out_d[:, c0:c1], outt[:, c0:c1])
        po.close()
        glob.close()
    split_multi_waits(nc, max_waits=1)
    return nc


# =============================== host side ==================================

def prep_core_inputs(inputs, b, half):
    import ml_dtypes
    f32 = np.float32
    bf16 = ml_dtypes.bfloat16
    d0 = half * HH

    def asf(a):
        return np.asarray(a, f32)

    bnscale = asf(inputs['bn_g']) / np.sqrt(np.float32(1.0 + 1e-5))
    w1 = asf(inputs['conv1_w'])[:, :, 0, 0] * bnscale[:, None]
    b1 = asf(inputs['conv1_b']) * bnscale + asf(inputs['bn_b'])

    def diag9(w, nch):
        out = np.zeros((nch, 9 * nch), f32)
        w = asf(w)
        for tap in range(9):
            dy, dx = tap // 3, tap % 3
            blk = out[:, tap * nch:(tap + 1) * nch]
            np.fill_diagonal(blk, w[:, 0, dy, dx])
        return out

    sscd = diag9(inputs['ss_conv_w'], DI)        # (192, 9*192)
    sc0 = np.zeros((128, 9 * 128), f32)
    sc1 = np.zeros((64, 9 * 64), f32)
    for tap in range(9):
        blk = sscd[:, tap * DI:(tap + 1) * DI]
        sc0[:, tap * 128:(tap + 1) * 128] = blk[0:128, 0:128]
        sc1[:, tap * 64:(tap + 1) * 64] = blk[128:192, 128:192]

    sel = np.zeros((DI, HH), f32)
    sel[np.arange(d0, d0 + HH), np.arange(HH)] = 1.0

    xp = asf(inputs['ss_xproj_w'])               # (4, 38, 192)
    xpTa = np.zeros((128, K4 * 38), f32)
    xpTb = np.zeros((64, K4 * 38), f32)
    for k in range(K4):
        xpT = xp[k].T                            # (192, 38)
        xpTa[:, k * 38:(k + 1) * 38] = xpT[0:128]
        xpTb[:, k * 38:(k + 1) * 38] = xpT[128:192]

    dtw = asf(inputs['ss_dt_w'])
    dtwT = np.zeros((R, K4 * HH), f32)
    for k in range(K4):
        dtwT[:, k * HH:(k + 1) * HH] = dtw[k][d0:d0 + HH, :].T

    dtb_full = asf(inputs['ss_dt_b'])
    Alog = asf(inputs['ss_Alog']).reshape(K4, DI, N)
    Dv = asf(inputs['ss_D']).reshape(K4, DI)
    dtb_p = np.zeros((128, NT), f32)
    Ap = np.zeros((128, NT * N), f32)
    for (j, o0, o1, k, dd0, dd1) in SECTIONS:
        dtb_p[o0:o1, j] = dtb_full[k, d0 + dd0:d0 + dd1]
        Ap[o0:o1, j * N:(j + 1) * N] = -np.exp(Alog[k, d0 + dd0:d0 + dd1])
    Dsum = Dv[:, d0:d0 + HH].sum(0)[:, None]

    gw = asf(inputs['gwc_w'])
    gT = np.zeros((24, 9 * CH), f32)
    for tap in range(9):
        dy, dx = tap // 3, tap % 3
        blk = np.zeros((24, CH), f32)
        blk[0:12, 0:48] = gw[0:48, :, dy, dx].T
        blk[12:24, 48:96] = gw[48:96, :, dy, dx].T
        gT[:, tap * CH:(tap + 1) * CH] = blk

    owT = asf(inputs['ss_out_w']).T              # (192, 96)
    outn_g = asf(inputs['ss_outn_g'])
    outn_b = asf(inputs['ss_outn_b'])

    vals32 = {
        'b1': b1[:, None],
        'linb': asf(inputs['lin_b'])[:, None],
        'dw1b': asf(inputs['dw1_b'])[:, None],
        'dw2b': asf(inputs['dw2_b'])[:, None],
        'scb0': asf(inputs['ss_conv_b'])[0:128, None],
        'scb1': asf(inputs['ss_conv_b'])[128:192, None],
        'dtb': dtb_p, 'Ap': Ap, 'Dsum': Dsum,
        'outngA': outn_g[0:96, None], 'outngB': outn_g[96:192, None],
        'outnbA': outn_b[0:96, None], 'outnbB': outn_b[96:192, None],
        'ag1b': asf(inputs['ag1_b'])[:, None],
        'ag2b': asf(inputs['ag2_b'])[:, None],
        'lng': asf(inputs['ln_g'])[:, None],
        'lnb': asf(inputs['ln_b'])[:, None],
        'gwcb': asf(inputs['gwc_b'])[:, None],
        'finb': asf(inputs['fin_b'])[:, None],
    }
    valsbf = {
        'w1T': w1.T,
        'linT': asf(inputs['lin_w']).T,
        'dw1dg': diag9(inputs['dw1_w'], CH),
        'dw2dg': diag9(inputs['dw2_w'], CH),
        'inwT': asf(inputs['ss_in_w']).T,        # (96, 384) full z
        'sc0dg': sc0, 'sc1dg': sc1,
        'sel0': sel[0:128], 'sel1': sel[128:192],
        'xpTa': xpTa, 'xpTb': xpTb,
        'dtwT': dtwT,
        'ident': np.eye(128, dtype=f32),
        'outwTa': owT[0:96], 'outwTb': owT[96:192],
        'ag1T': asf(inputs['ag1_w'])[:, :, 0, 0].T,
        'ag2T': asf(inputs['ag2_w'])[:, :, 0, 0].T,
        'sq1T': asf(inputs['sq1_w'])[:, :, 0, 0].T,
        'sq2T': asf(inputs['sq2_w'])[:, :, 0, 0].T,
        'gwcT': gT,
        'pw1T': asf(inputs['pwc1_w'])[:, :, 0, 0].T,
        'pw2T': asf(inputs['pwc2_w'])[:, :, 0, 0].T,
        'finT': asf(inputs['fin_w']).T,
        'bcv': np.concatenate([np.full((1, 96), 1.0 / DI, f32),
                               np.full((1, 96), 1.0 / CH, f32),
                               np.ones((1, 96), f32)], axis=1),
    }

    blob32 = np.zeros((128, W32), f32)
    for nm, p, c in CONSTS_F32:
        o = OFF32[nm][0]
        v = vals32[nm]
        assert v.shape == (p, c), (nm, v.shape, (p, c))
        blob32[0:p, o:o + c] = v
    blobbf = np.zeros((128, WBF), bf16)
    for nm, p, c in CONSTS_BF16:
        o = OFFBF[nm][0]
        v = valsbf[nm]
        assert v.shape == (p, c), (nm, v.shape, (p, c))
        blobbf[0:p, o:o + c] = v.astype(bf16)

    return {
        'x': np.ascontiguousarray(asf(inputs['x'])[b].reshape(CIN, L).astype(bf16)),
        'c32': blob32,
        'cbf': np.ascontiguousarray(blobbf),
    }


_NC_CACHE = {}


def get_nc():
    if 'nc' not in _NC_CACHE:
        _NC_CACHE['nc'] = build_nc()
    return _NC_CACHE['nc']


def kernel(**inputs):
    from concourse.bass_utils import run_bass_kernel_spmd
    nc = get_nc()
    in_maps = [prep_core_inputs(inputs, c // 2, c % 2) for c in range(8)]
    res = run_bass_kernel_spmd(nc, in_maps, core_ids=list(range(8)))
    out = np.zeros((B_, COUT, H, W), np.float32)
    for b in range(B_):
        out[b] = res.results[2 * b]['out'].reshape(COUT, H, W)
    return out


# revision 62
# speedup vs baseline: 1.0618x; 1.0005x over previous
"""Self-contained Trainium2 Bass kernel for the CR-VSS block (8 cores)."""

# ---- TileContext drain-wait patch (walrus 1-wait limit) ----
"""Patch TileContext._drain_and_barrier: the axon-client walrus rejects
instructions carrying >2 sem waits ("Too many sync wait commands" in
setupSyncWait for CTRL structs). Redistribute the exit-drain's waits across
preceding SP nop instructions, each carrying at most MAX_WAITS."""
from concourse.tile import TileContext, ScopedClock

MAX_WAITS = 1


def _patched_drain_and_barrier(self, tick_clock, wait_clock):
    nc = self.nc
    drain_inst = nc.sync.drain()
    wait_clock.add_sem_waits(
        drain_inst.ins, ScopedClock({None: tick_clock.global_clock})
    )

    waits = list(drain_inst.ins.sync_info.on_wait or [])
    if len(waits) > MAX_WAITS:
        bb = nc.cur_bb.bb
        assert bb.instructions[-1] is drain_inst.ins
        # strip waits from the drain, re-emit them on nop carriers
        drain_inst.ins.sync_info.on_wait = waits[:0]
        carriers = []
        import concourse.mybir as mybir
        for i in range(0, len(waits), MAX_WAITS):
            nop = nc.sync.nop(nofuse=True)
            nop.ins.sync_info = mybir.SyncInfo(
                on_wait=waits[i:i + MAX_WAITS], on_update=[]
            )
            carriers.append(nop.ins)
        # move carriers before the drain
        insts = list(bb.instructions)
        assert insts[-len(carriers) - 1] is drain_inst.ins
        reordered = insts[:-len(carriers) - 1] + insts[-len(carriers):] + [drain_inst.ins]
        while len(bb.instructions):
            bb.instructions.pop()
        for x in reordered:
            bb.instructions.append(x)

    nc.all_engine_barrier()
    assert self.sems is not None
    popped = nc._tile_sem_poison_stack.pop()
    assert popped is self._sem_poison
    nc.clear_and_free_semaphores(list(self.sems.allocated().values()))
    nc.all_engine_barrier()


def apply():
    TileContext._drain_and_barrier = _patched_drain_and_barrier


def split_multi_waits(nc, max_waits=1):
    """Post-pass: walrus CTRL codegen rejects instructions with more than
    one sem wait. Move extra waits onto same-engine NoOp carriers."""
    import concourse.mybir as mybir
    for f in nc.m.functions:
        for bb in f.blocks:
            insts = list(bb.instructions)
            out = []
            changed = False
            for ins in insts:
                si = ins.sync_info
                if si is not None and si.on_wait and len(si.on_wait) > max_waits:
                    waits = list(si.on_wait)
                    for i, w in enumerate(waits[max_waits:]):
                        nop = mybir.InstNoOp.__new__(
                            mybir.InstNoOp, name=f"{ins.name}-xw{i}", ins=[], outs=[])
                        nop.engine = ins.engine
                        nop.sync_info = mybir.SyncInfo(on_wait=[w], on_update=[])
                        out.append(nop)
                    ins.sync_info = mybir.SyncInfo(
                        on_wait=waits[:max_waits],
                        on_update=list(si.on_update or []))
                    changed = True
                out.append(ins)
            if changed:
                while len(bb.instructions):
                    bb.instructions.pop()
                for x in out:
                    bb.instructions.append(x)

apply()

# ---- kernel ----
"""Trainium2 Bass kernel for nn_CR_VSS (VSS block with SS2D selective scan).

Sharding: 8 cores = 4 samples x 2 d_inner-halves. Each core runs the full
pre-stage for its sample, scans its 96-channel d-half across all 4
cross-scan directions (packed into 3x128-partition tiles), then the pair
exchanges y-halves with ONE AllGather; LN + out-proj + post-stage run
locally (z is computed full-width in the in-proj so no second collective).

Scan: h_t = exp(A*delta_t)*h_{t-1} + delta_t*u_t*B_t per (k,d,n) via
tensor_tensor_scan; n in groups of 4 with batched B/C partition-broadcast
DMAs (double-buffered); y accumulated over n with identity-lhsT PSUM
matmuls, merged into pixel-order ysum straight from PSUM per t-chunk.
"""
import numpy as np
from contextlib import ExitStack

import concourse.bass as bass
import concourse.mybir as mybir

F = mybir.ActivationFunctionType
A = mybir.AluOpType
FP32 = mybir.dt.float32
BF16 = mybir.dt.bfloat16

B_, CIN, CH, COUT, H, W = 4, 96, 96, 96, 48, 48
DI, N, R, K4 = 192, 16, 6, 4
L = H * W               # 2304
HH = 96                 # d-half per core
NT = 3                  # packed (k,d) tiles: 4*96 = 384 = 3*128
HP = 50
LP = 2500
TC = 768                # scan t-chunk (16 rows of 48)
TCH = [(0, 768), (768, 1536), (1536, 2304)]
NG = 4                  # scan n-group (broadcast batch)

# packed (k,d) rows -> (tile j, offset): sections (j, o0, o1, k, d0, d1).
# Section offsets are all 0/32/64 so PE matmuls can write them directly.
SECTIONS = [
    (0, 0, 32, 1, 0, 32),
    (0, 32, 128, 0, 0, 96),
    (1, 0, 64, 1, 32, 96),
    (1, 64, 128, 2, 0, 64),
    (2, 0, 32, 2, 64, 96),
    (2, 32, 128, 3, 0, 96),
]

MM_CHUNKS = [(0, 512), (512, 1024), (1024, 1536), (1536, 2048), (2048, 2304)]
ROW_CHUNKS = [(0, 10), (10, 20), (20, 30), (30, 40), (40, 48)]
SUBS768 = [(0, 512), (512, 768)]
INW_BLOCKS = [(0, 128), (128, 256), (256, 384)]

REPLICA_GROUPS = [[0, 1], [2, 3], [4, 5], [6, 7]]

# ---- const blobs (shared layout between host packing and kernel views) ----
CONSTS_F32 = [
    ('b1', 96, 1), ('linb', 96, 1),
    ('dw1b', 96, 1), ('dw2b', 96, 1),
    ('scb0', 128, 1), ('scb1', 64, 1),
    ('dtb', 128, 3), ('Ap', 128, 48), ('Dsum', 96, 1),
    ('outngA', 96, 1), ('outngB', 96, 1), ('outnbA', 96, 1), ('outnbB', 96, 1),
    ('ag1b', 48, 1), ('ag2b', 96, 1), ('lng', 96, 1), ('lnb', 96, 1),
    ('gwcb', 96, 1), ('finb', 96, 1),
]
CONSTS_BF16 = [
    ('w1T', 96, 96), ('linT', 96, 96),
    ('dw1dg', 96, 864), ('dw2dg', 96, 864),
    ('inwT', 96, 384),
    ('sc0dg', 128, 1152), ('sc1dg', 64, 576),
    ('sel0', 128, 96), ('sel1', 64, 96),
    ('xpTa', 128, 152), ('xpTb', 64, 152),
    ('dtwT', 6, 384),
    ('ident', 128, 128),
    ('outwTa', 96, 96), ('outwTb', 96, 96),
    ('ag1T', 96, 48), ('ag2T', 48, 96),
    ('sq1T', 48, 24), ('sq2T', 48, 24),
    ('gwcT', 24, 864), ('pw1T', 24, 96), ('pw2T', 24, 72),
    ('finT', 96, 96), ('bcv', 1, 288),
]

OFF32 = {}
_o = 0
for _nm, _p, _c in CONSTS_F32:
    OFF32[_nm] = (_o, _p, _c)
    _o += _c
W32 = _o
OFFBF = {}
_o = 0
for _nm, _p, _c in CONSTS_BF16:
    OFFBF[_nm] = (_o, _p, _c)
    _o += _c
WBF = _o


def build_nc():
    nc = bass.Bass(trn_type="TRN2", num_devices=8)

    x_d = nc.dram_tensor("x", [CIN, L], BF16, kind="ExternalInput")
    c32_d = nc.dram_tensor("c32", [128, W32], FP32, kind="ExternalInput")
    cbf_d = nc.dram_tensor("cbf", [128, WBF], BF16, kind="ExternalInput")
    out_d = nc.dram_tensor("out", [COUT, L], FP32, kind="ExternalOutput")

    B_dram = nc.dram_tensor("B_dram", [K4 * N, L], BF16)
    C_dram = nc.dram_tensor("C_dram", [K4 * N, L], BF16)
    y_dram = nc.dram_tensor("y_dram", [HH, L], BF16)
    yg_dram = nc.dram_tensor("yg_dram", [DI, L], BF16)
    st_dram = nc.dram_tensor("st_dram", [2, L], BF16)

    def hw(ap):
        return ap.rearrange("p (h w) -> p h w", h=H)

    def hwp(ap):
        return ap.rearrange("p (h w) -> p h w", h=HP)

    def whv(ap):
        return ap.rearrange("p (h w) -> p w h", h=H)

    with TileContext(nc) as tc:
        glob = ExitStack()
        cst = glob.enter_context(tc.tile_pool(name="cst", bufs=1))
        lngA = glob.enter_context(tc.tile_pool(name="lngA", bufs=1))

        cst32 = cst.tile([128, W32], FP32, tag="cst32")
        cstbf = cst.tile([128, WBF], BF16, tag="cstbf")
        nc.sync.dma_start(cst32[:], c32_d[:])
        nc.sync.dma_start(cstbf[:], cbf_d[:])

        def cvc(nm, a0=0, a1=None, p0=0, p1=None):
            d, tile = (OFF32, cst32) if nm in OFF32 else (OFFBF, cstbf)
            o, p, c = d[nm]
            if a1 is None:
                a1 = c
            if p1 is None:
                p1 = p
            return tile[p0:p1, o + a0:o + a1]

        cv = cvc

        ones96 = cst.tile([HH, 1], BF16, tag="ones96")
        nc.vector.memset(ones96[:], 1.0)

        # long-lived across phases
        z0 = lngA.tile([128, L], BF16, tag="z0")     # z rows 0:128
        z1 = lngA.tile([64, L], BF16, tag="z1")      # z rows 128:192
        x2 = lngA.tile([CH, L], BF16, tag="x2")
        lngB = ExitStack()
        lngB_p = lngB.enter_context(tc.tile_pool(name="lngB_p", bufs=1))
        xch = lngB_p.tile([HH, L], BF16, tag="xch")
        dp = [lngB_p.tile([128, L], BF16, tag=f"dp{j}", name=f"dp{j}") for j in range(NT)]
        # xsp holds packed scan-order xs, overwritten in place with delta*u
        xsp = [lngB_p.tile([128, L], BF16, tag=f"xsp{j}", name=f"xsp{j}") for j in range(NT)]
        ysum = lngB_p.tile([HH, L], BF16, tag="ysum")

        # ================= pre-stage =================
        pre = ExitStack()
        pre_ps = pre.enter_context(tc.tile_pool(name="pre_ps", bufs=4, space="PSUM"))
        pA = pre.enter_context(tc.tile_pool(name="pA", bufs=1))
        pB = pre.enter_context(tc.tile_pool(name="pB", bufs=1))

        xt = pA.tile([CIN, L], BF16, tag="xt")
        nc.sync.dma_start(xt[:], x_d[:])

        # conv1x1 (+folded BN) + ReLU
        h1 = pA.tile([CH, L], BF16, tag="h1")
        for c0, c1 in MM_CHUNKS:
            ps = pre_ps.tile([CH, 512], FP32, tag="ps")
            nc.tensor.matmul(ps[:, :c1 - c0], cv('w1T'), xt[:, c0:c1], start=True, stop=True)
            nc.scalar.activation(h1[:, c0:c1], ps[:, :c1 - c0], F.Relu, bias=cv('b1'))
        # token linear
        h2 = pA.tile([CH, L], BF16, tag="h2")
        for c0, c1 in MM_CHUNKS:
            ps = pre_ps.tile([CH, 512], FP32, tag="ps")
            nc.tensor.matmul(ps[:, :c1 - c0], cv('linT'), h1[:, c0:c1], start=True, stop=True)
            nc.vector.tensor_scalar(out=h2[:, c0:c1], in0=ps[:, :c1 - c0],
                                    scalar1=cv('linb'), scalar2=None, op0=A.add)
        h2p = pA.tile([CH, LP], BF16, tag="h2p")
        nc.gpsimd.memset(h2p[:], 0.0)
        for (r0, r1) in ROW_CHUNKS:
            nc.vector.tensor_copy(hwp(h2p[:])[:, r0 + 1:r1 + 1, 1:49],
                                  hw(h2[:])[:, r0:r1, :])

        def dwconv(dst, src_p, dgname, biasname, nch):
            for (r0, r1) in ROW_CHUNKS:
                nr = r1 - r0
                ps = pre_ps.tile([128, 480], FP32, tag="ps")
                for tap in range(9):
                    dy, dx = tap // 3, tap % 3
                    rhs = hwp(src_p[:])[:, dy + r0:dy + r1, dx:dx + 48]
                    nc.tensor.matmul(ps[:nch, :nr * 48],
                                     cvc(dgname, tap * nch, (tap + 1) * nch),
                                     rhs, start=(tap == 0), stop=(tap == 8))
                nc.scalar.activation(dst[:, r0 * 48:r1 * 48], ps[:nch, :nr * 48],
                                     F.Silu, bias=cv(biasname))

        x1 = pB.tile([CH, L], BF16, tag="x1")
        dwconv(x1, h2p, 'dw1dg', 'dw1b', CH)

        # in-proj: xi (192) + FULL z (192)
        xi0 = pB.tile([128, L], BF16, tag="xi0")
        xi1 = pB.tile([64, L], BF16, tag="xi1")
        for mb, (m0, m1) in enumerate(INW_BLOCKS):
            for c0, c1 in MM_CHUNKS:
                ps = pre_ps.tile([128, 512], FP32, tag="ps")
                nc.tensor.matmul(ps[:m1 - m0, :c1 - c0], cvc('inwT', m0, m1),
                                 x1[:, c0:c1], start=True, stop=True)
                if mb == 0:
                    nc.vector.tensor_copy(xi0[:, c0:c1], ps[:128, :c1 - c0])
                elif mb == 1:
                    nc.scalar.copy(xi1[:, c0:c1], ps[0:64, :c1 - c0])
                    nc.scalar.copy(z0[0:64, c0:c1], ps[64:128, :c1 - c0])
                else:
                    nc.scalar.copy(z0[64:128, c0:c1], ps[0:64, :c1 - c0])
                    nc.scalar.copy(z1[0:64, c0:c1], ps[64:128, :c1 - c0])

        xi0p = pB.tile([128, LP], BF16, tag="xi0p")
        xi1p = pB.tile([64, LP], BF16, tag="xi1p")
        nc.gpsimd.memset(xi0p[:], 0.0)
        nc.gpsimd.memset(xi1p[:], 0.0)
        for (r0, r1) in ROW_CHUNKS:
            nc.vector.tensor_copy(hwp(xi0p[:])[:, r0 + 1:r1 + 1, 1:49],
                                  hw(xi0[:])[:, r0:r1, :])
            nc.vector.tensor_copy(hwp(xi1p[:])[:, r0 + 1:r1 + 1, 1:49],
                                  hw(xi1[:])[:, r0:r1, :])
        xc0 = pB.tile([128, L], BF16, tag="xc0")
        xc1 = pB.tile([64, L], BF16, tag="xc1")
        dwconv(xc0, xi0p, 'sc0dg', 'scb0', 128)
        dwconv(xc1, xi1p, 'sc1dg', 'scb1', 64)

        # d-half extraction + wh copy
        for c0, c1 in MM_CHUNKS:
            ps = pre_ps.tile([HH, 512], FP32, tag="ps")
            nc.tensor.matmul(ps[:, :c1 - c0], cv('sel0'), xc0[:, c0:c1], start=True, stop=False)
            nc.tensor.matmul(ps[:, :c1 - c0], cv('sel1'), xc1[:, c0:c1], start=False, stop=True)
            nc.vector.tensor_copy(xch[:, c0:c1], ps[:, :c1 - c0])
        xwhh = pB.tile([HH, L], BF16, tag="xwhh")
        for (t0, t1) in TCH:
            w0, w1 = t0 // 48, t1 // 48
            nc.vector.tensor_copy(hw(xwhh[:])[:, w0:w1, :],
                                  whv(xch[:])[:, w0:w1, :])

        # xproj (compact 38 rows: 0:6 dts, 6:22 B, 22:38 C) in scan order
        def xc_read(k, c0, c1):
            if k == 0:
                return (xc0[:, c0:c1], xc1[:, c0:c1])
            if k == 1:
                return (whv(xc0[:])[:, c0 // 48:c1 // 48, :],
                        whv(xc1[:])[:, c0 // 48:c1 // 48, :])
            if k == 2:
                return (xc0[:, L - c1:L - c0][:, ::-1],
                        xc1[:, L - c1:L - c0][:, ::-1])
            r0 = whv(xc0[:])[:, (L - c1) // 48:(L - c0) // 48, :][:, ::-1, ::-1]
            r1 = whv(xc1[:])[:, (L - c1) // 48:(L - c0) // 48, :][:, ::-1, ::-1]
            return (r0, r1)

        # row-chunk outer so all 4 directions' early columns finish first;
        # B/C are written to DRAM per scan chunk so ci=0 broadcasts can
        # start while xproj still works on later chunks.
        stage = [pB.tile([38, L], BF16, tag=f"stg{k}", name=f"stg{k}") for k in range(K4)]
        done_w = 0
        for ri, (rr0, rr1) in enumerate(ROW_CHUNKS):
            c0, c1 = rr0 * 48, rr1 * 48
            nf = c1 - c0
            for k in range(K4):
                ra, rb = xc_read(k, c0, c1)
                ps = pre_ps.tile([38, 480], FP32, tag="ps")
                nc.tensor.matmul(ps[:, :nf], cvc('xpTa', k * 38, (k + 1) * 38), ra,
                                 start=True, stop=False)
                nc.tensor.matmul(ps[:, :nf], cvc('xpTb', k * 38, (k + 1) * 38), rb,
                                 start=False, stop=True)
                nc.vector.tensor_copy(stage[k][:, c0:c1], ps[:, :nf])
            while done_w < len(TCH) and TCH[done_w][1] <= c1:
                t0, t1 = TCH[done_w]
                for k in range(K4):
                    nc.sync.dma_start(B_dram[k * N:(k + 1) * N, t0:t1],
                                      stage[k][6:22, t0:t1])
                    nc.sync.dma_start(C_dram[k * N:(k + 1) * N, t0:t1],
                                      stage[k][22:38, t0:t1])
                done_w += 1

        # delta: packed matmuls then softplus on full 128-partition tiles
        def mm_windows(a0, a1):
            if a0 == 0:
                return [(0, a1)]
            res = []
            x = a0
            while x < a1:
                if x % 64 == 32:
                    e = min(a1, x + 32)
                else:  # x == 64
                    e = min(a1, 128)
                res.append((x, e))
                x = e
            return res

        for (cc0, cc1) in MM_CHUNKS:
            cw = cc1 - cc0
            for j in range(NT):
                ex = pre_ps.tile([128, 512], FP32, tag="ps")
                for (jj, o0, o1, k, d0, d1) in SECTIONS:
                    if jj != j:
                        continue
                    for (w0, w1) in mm_windows(o0, o1):
                        dd0 = d0 + (w0 - o0)
                        dd1 = d0 + (w1 - o0)
                        nc.tensor.matmul(ex[w0:w1, :cw],
                                         cvc('dtwT', k * 96 + dd0, k * 96 + dd1),
                                         stage[k][0:6, cc0:cc1], start=True, stop=True)
                # softplus(x+b) = ln(1 + exp(x+b)) (no softplus act table on HW)
                ex2 = pre_ps.tile([128, 512], FP32, tag="ps")
                nc.scalar.activation(ex2[:, :cw], ex[:, :cw], F.Exp,
                                     bias=cvc('dtb', j, j + 1))
                nc.scalar.activation(dp[j][:, cc0:cc1], ex2[:, :cw], F.Ln, bias=1.0)

        # pack scan-order xs (Act copies handle partition shift + flips),
        # then overwrite in place with delta*u = dp*xs.
        # Act partition windows must not cross engine block boundaries on
        # EITHER side: allowed starts 0/32/64/96; a start-32 window may not
        # cross 64. split2 chops a shifted copy accordingly.
        def _legal_span(s):
            return 32 if s == 32 else 128 - s if s else 128

        def split2(o0, i0, ln):
            res = []
            x = 0
            while x < ln:
                step = min(ln - x, _legal_span(o0 + x), _legal_span(i0 + x))
                res.append((x, x + step))
                x += step
            return res

        for (t0, t1) in TCH:
            for (j, o0, o1, k, d0, d1) in SECTIONS:
                v = xwhh if k in (1, 3) else xch
                if k < 2:
                    # forward sections: contiguous rows, cheap DMA shift
                    nc.sync.dma_start(xsp[j][o0:o1, t0:t1], v[d0:d1, t0:t1])
                    continue
                for (w0, w1) in split2(o0, d0, o1 - o0):
                    nc.scalar.copy(xsp[j][o0 + w0:o0 + w1, t0:t1],
                                   v[d0 + w0:d0 + w1, ::-1][:, t0:t1])
            for j in range(NT):
                nc.vector.tensor_mul(xsp[j][:, t0:t1], dp[j][:, t0:t1],
                                     xsp[j][:, t0:t1])

        # branch-2 dwconv last: x2 is only needed in the post stage, so
        # this fills PE/Act slack once the scan inputs are queued
        dwconv(x2, h2p, 'dw2dg', 'dw2b', CH)

        pre.close()

        # ================= scan =================
        sc = ExitStack()
        bbp = sc.enter_context(tc.tile_pool(name="bbp", bufs=2))
        spool = sc.enter_context(tc.tile_pool(name="spool", bufs=2))
        scan_ps = sc.enter_context(tc.tile_pool(name="scan_ps", bufs=1, space="PSUM"))
        stp = sc.enter_context(tc.tile_pool(name="stp", bufs=1))
        state = [stp.tile([128, N], FP32, tag=f"st{j}", name=f"state{j}")
                 for j in range(NT)]
        yd = [stp.tile([HH, L], BF16, tag=f"yd{k}", name=f"yd{k}") for k in range(K4)]

        pending_drain = None
        for ci, (c0, c1) in enumerate(TCH):
            ypsum = [scan_ps.tile([128, TC], FP32, tag=f"yps{j}", name=f"yps{j}_{ci}")
                     for j in range(NT)]
            for g in range(K4):
                Bb = [bbp.tile([128, NG * TC], BF16, tag=f"Bb{j}", name=f"Bb{j}_{ci}_{g}")
                      for j in range(NT)]
                Cb = [bbp.tile([128, NG * TC], BF16, tag=f"Cb{j}", name=f"Cb{j}_{ci}_{g}")
                      for j in range(NT)]
                for (j, o0, o1, k, d0, d1) in SECTIONS:
                    nc.sync.dma_start(
                        Bb[j][o0:o1, :],
                        B_dram[k * N + NG * g:k * N + NG * g + NG, c0:c1]
                        .partition_broadcast(o1 - o0))
                    nc.scalar.dma_start(
                        Cb[j][o0:o1, :],
                        C_dram[k * N + NG * g:k * N + NG * g + NG, c0:c1]
                        .partition_broadcast(o1 - o0))
                if pending_drain is not None:
                    pending_drain()
                    pending_drain = None
                for n4 in range(NG):
                    n = NG * g + n4
                    for j in range(NT):
                        at = spool.tile([128, TC], BF16, tag=f"at{j}", name=f"at{j}")
                        nc.scalar.activation(at[:], dp[j][:, c0:c1], F.Exp,
                                             scale=cvc('Ap', j * N + n, j * N + n + 1))
                        bt = spool.tile([128, TC], BF16, tag=f"bt{j}", name=f"bt{j}")
                        nc.vector.tensor_mul(bt[:], xsp[j][:, c0:c1],
                                             Bb[j][:, n4 * TC:(n4 + 1) * TC])
                        ht = spool.tile([128, TC], BF16, tag=f"ht{j}", name=f"ht{j}")
                        if ci > 0:
                            # fold carry state into bt[0] so the scan can use
                            # the cheap zero-init form
                            nc.vector.scalar_tensor_tensor(
                                out=bt[:, 0:1], in0=at[:, 0:1],
                                scalar=state[j][:, n:n + 1], in1=bt[:, 0:1],
                                op0=A.mult, op1=A.add)
                        nc.vector.tensor_tensor_scan(ht[:], at[:], bt[:], 0.0,
                                                     A.mult, A.add)
                        if ci < 2:
                            nc.vector.tensor_copy(state[j][:, n:n + 1], ht[:, TC - 1:TC])
                        gt = spool.tile([128, TC], BF16, tag=f"gt{j}", name=f"gt{j}")
                        nc.vector.tensor_mul(gt[:], ht[:],
                                             Cb[j][:, n4 * TC:(n4 + 1) * TC])
                        for (s0, s1) in SUBS768:
                            nc.tensor.matmul(ypsum[j][:, s0:s1], cv('ident'),
                                             gt[:, s0:s1],
                                             start=(n == 0), stop=(n == N - 1))
            # drain this chunk's PSUM into per-direction scan-order tiles
            # (Act copies allow the partition shift). Deferred past the next
            # chunk's broadcast issue so the boundary doesn't stall Act.
            def _drain(yps=ypsum, cc0=c0, cc1=c1):
                for (j, o0, o1, k, d0, d1) in SECTIONS:
                    for (w0, w1) in split2(d0, o0, d1 - d0):
                        nc.scalar.copy(yd[k][d0 + w0:d0 + w1, cc0:cc1],
                                       yps[j][o0 + w0:o0 + w1, :])
            pending_drain = _drain
        if pending_drain is not None:
            pending_drain()
            pending_drain = None
        # merge directions into pixel order + D*u term
        tmp96 = stp.tile([HH, L], BF16, tag="tmp96")
        nc.vector.tensor_add(ysum[:], yd[0][:], yd[2][:, ::-1])
        nc.vector.tensor_add(tmp96[:], yd[1][:], yd[3][:, ::-1])
        nc.vector.tensor_add(ysum[:], ysum[:], whv(tmp96[:]))
        nc.vector.scalar_tensor_tensor(out=ysum[:], in0=xch[:], scalar=cv('Dsum'),
                                       in1=ysum[:], op0=A.mult, op1=A.add)
        nc.sync.dma_start(y_dram[:], ysum[:])
        nc.gpsimd.collective_compute(
            "AllGather", A.bypass, replica_groups=REPLICA_GROUPS,
            ins=[y_dram[:]], outs=[yg_dram[:]])
        sc.close()
        lngB.close()

        # ================= post =================
        po = ExitStack()
        post_ps = po.enter_context(tc.tile_pool(name="post_ps", bufs=4, space="PSUM"))
        pP = po.enter_context(tc.tile_pool(name="pP", bufs=1))
        rot = po.enter_context(tc.tile_pool(name="rot", bufs=4))

        # branch 2 + silu(z): no dep on the collective, runs under it
        g1 = rot.tile([48, L], BF16, tag="pb")
        for c0, c1 in MM_CHUNKS:
            ps = post_ps.tile([48, 512], FP32, tag="ps")
            nc.tensor.matmul(ps[:, :c1 - c0], cv('ag1T'), x2[:, c0:c1], start=True, stop=True)
            nc.scalar.activation(g1[:, c0:c1], ps[:, :c1 - c0], F.Relu, bias=cv('ag1b'))
        gat = rot.tile([CH, L], BF16, tag="pb")
        for c0, c1 in MM_CHUNKS:
            ps = post_ps.tile([CH, 512], FP32, tag="ps")
            nc.tensor.matmul(ps[:, :c1 - c0], cv('ag2T'), g1[:, c0:c1], start=True, stop=True)
            nc.scalar.activation(gat[:, c0:c1], ps[:, :c1 - c0], F.Sigmoid, bias=cv('ag2b'))
        x2g = pP.tile([CH, L], BF16, tag="x2g")
        nc.vector.tensor_mul(x2g[:], x2[:], gat[:])

        zsA = pP.tile([HH, L], BF16, tag="zsA")
        zsB = pP.tile([HH, L], BF16, tag="zsB")
        nc.scalar.activation(zsA[:], z0[0:96, :], F.Silu)
        nc.scalar.activation(zsB[0:32, :], z0[96:128, :], F.Silu)
        nc.scalar.activation(zsB[32:64, :], z1[0:32, :], F.Silu)
        nc.scalar.activation(zsB[64:96, :], z1[32:64, :], F.Silu)

        ygA = pP.tile([HH, L], BF16, tag="ygA")
        ygB = pP.tile([HH, L], BF16, tag="ygB")
        nc.sync.dma_start(ygA[:], yg_dram[0:HH, :])
        nc.sync.dma_start(ygB[:], yg_dram[HH:DI, :])

        sA = pP.tile([1, L], BF16, tag="sA")
        sB = pP.tile([1, L], BF16, tag="sB")
        sM = pP.tile([1, L], BF16, tag="sM")

        def ln_stats(cinv):
            # in: sA=raw sum, sB=raw sumsq; leaves rstd in sB (sA stays raw sum)
            with nc.allow_low_precision(reason="LN stats kept bf16; rel-err verified"):
                nc.scalar.activation(sM[:], sA[:], F.Square, scale=cinv)
                nc.vector.scalar_tensor_tensor(out=sB[:], in0=sB[:], scalar=cinv,
                                               in1=sM[:], op0=A.mult, op1=A.subtract)
                nc.vector.tensor_scalar(out=sB[:], in0=sB[:], scalar1=1e-5,
                                        scalar2=None, op0=A.add)
                nc.vector.reciprocal(sB[:], sB[:])
                nc.scalar.activation(sB[:], sB[:], F.Sqrt)

        # LayerNorm over full DI (local stats via ones-matmul), fully
        # chunk-pipelined behind the per-chunk yg reads
        ysqA = rot.tile([HH, L], BF16, tag="pb")
        ysqB = rot.tile([HH, L], BF16, tag="pb")
        for c, (c0, c1) in enumerate(MM_CHUNKS):
            nc.vector.tensor_mul(ysqA[:, c0:c1], ygA[:, c0:c1], ygA[:, c0:c1])
            nc.vector.tensor_mul(ysqB[:, c0:c1], ygB[:, c0:c1], ygB[:, c0:c1])
            ps = post_ps.tile([1, 512], FP32, tag="ps")
            nc.tensor.matmul(ps[:, :c1 - c0], ones96[:], ygA[:, c0:c1], start=True, stop=False)
            nc.tensor.matmul(ps[:, :c1 - c0], ones96[:], ygB[:, c0:c1], start=False, stop=True)
            nc.scalar.copy(sA[0:1, c0:c1], ps[:, :c1 - c0])
            ps2 = post_ps.tile([1, 512], FP32, tag="ps")
            nc.tensor.matmul(ps2[:, :c1 - c0], ones96[:], ysqA[:, c0:c1], start=True, stop=False)
            nc.tensor.matmul(ps2[:, :c1 - c0], ones96[:], ysqB[:, c0:c1], start=False, stop=True)
            nc.scalar.copy(sB[0:1, c0:c1], ps2[:, :c1 - c0])

        ln_stats(1.0 / DI)

        def apply_ln(pairs, bco):
            # pairs: list of (dst, src, gname, bname); bco: bcv column offset
            # holding 1/DI or 1/CH (folds the mean division into the
            # broadcast lhsT)
            for c0, c1 in MM_CHUNKS:
                cw = c1 - c0
                psm = post_ps.tile([HH, 512], FP32, tag="ps")
                nc.tensor.matmul(psm[:, :cw], cvc('bcv', bco, bco + HH),
                                 sA[:, c0:c1], start=True, stop=True)
                psr = post_ps.tile([HH, 512], FP32, tag="ps")
                nc.tensor.matmul(psr[:, :cw], cvc('bcv', 192, 192 + HH),
                                 sB[:, c0:c1], start=True, stop=True)
                for (dst, srct, gname, bname) in pairs:
                    nc.vector.tensor_sub(dst[:, c0:c1], srct[:, c0:c1],
                                         psm[:, :cw])
                    nc.vector.tensor_mul(dst[:, c0:c1], dst[:, c0:c1],
                                         psr[:, :cw])
                    nc.vector.tensor_scalar(out=dst[:, c0:c1], in0=dst[:, c0:c1],
                                            scalar1=cv(gname), scalar2=cv(bname),
                                            op0=A.mult, op1=A.add)

        ynA = rot.tile([HH, L], BF16, tag="pb")
        ynB = rot.tile([HH, L], BF16, tag="pb")
        apply_ln([(ynA, ygA, 'outngA', 'outnbA'),
                  (ynB, ygB, 'outngB', 'outnbB')], 0)

        # fused chunk pipeline: ygz -> out-proj -> yb -> ybsq -> LN2 stats
        gzA = rot.tile([HH, L], BF16, tag="pb")
        gzB = rot.tile([HH, L], BF16, tag="pb")
        x1o = pP.tile([CH, L], BF16, tag="x1o")
        yb = pP.tile([CH, L], BF16, tag="yb")
        ybsq = rot.tile([CH, L], BF16, tag="pb")
        for c0, c1 in MM_CHUNKS:
            cw = c1 - c0
            nc.vector.tensor_mul(gzA[:, c0:c1], ynA[:, c0:c1], zsA[:, c0:c1])
            nc.vector.tensor_mul(gzB[:, c0:c1], ynB[:, c0:c1], zsB[:, c0:c1])
            ps = post_ps.tile([CH, 512], FP32, tag="ps")
            nc.tensor.matmul(ps[:, :cw], cv('outwTa'), gzA[:, c0:c1],
                             start=True, stop=False)
            nc.tensor.matmul(ps[:, :cw], cv('outwTb'), gzB[:, c0:c1],
                             start=False, stop=True)
            nc.scalar.copy(x1o[:, c0:c1], ps[:, :cw])
            nc.vector.tensor_add(yb[:, c0:c1], x1o[:, c0:c1], x2g[:, c0:c1])
            nc.vector.tensor_mul(ybsq[:, c0:c1], yb[:, c0:c1], yb[:, c0:c1])
            ps1 = post_ps.tile([1, 512], FP32, tag="ps")
            nc.tensor.matmul(ps1[:, :cw], ones96[:], yb[:, c0:c1], start=True, stop=True)
            nc.scalar.copy(sA[0:1, c0:c1], ps1[:, :cw])
            ps2 = post_ps.tile([1, 512], FP32, tag="ps")
            nc.tensor.matmul(ps2[:, :cw], ones96[:], ybsq[:, c0:c1], start=True, stop=True)
            nc.scalar.copy(sB[0:1, c0:c1], ps2[:, :cw])
        ln_stats(1.0 / CH)
        ybn = pP.tile([CH, L], BF16, tag="ybn")
        apply_ln([(ybn, yb, 'lng', 'lnb')], 96)

        # CRM
        low_t = rot.tile([48, L], BF16, tag="pb")
        for c0, c1 in MM_CHUNKS:
            nc.sync.dma_start(low_t[:, c0:c1], ybn[48:96, c0:c1])
        upc = pP.tile([24, L], BF16, tag="upc")
        lowc = pP.tile([24, L], BF16, tag="lowc")
        m2cb = pP.tile([24, 5], FP32, tag="m2cb")
        _li = {c0: i for i, (c0, c1) in enumerate(MM_CHUNKS)}.get
        for c0, c1 in MM_CHUNKS:
            ps = post_ps.tile([24, 512], FP32, tag="ps")
            nc.tensor.matmul(ps[:, :c1 - c0], cv('sq1T'), ybn[0:48, c0:c1], start=True, stop=True)
            nc.scalar.copy(upc[:, c0:c1], ps[:, :c1 - c0])
            ps2 = post_ps.tile([24, 512], FP32, tag="ps")
            nc.tensor.matmul(ps2[:, :c1 - c0], cv('sq2T'), low_t[:, c0:c1], start=True, stop=True)
            nc.scalar.activation(lowc[:, c0:c1], ps2[:, :c1 - c0], F.Identity,
                                 accum_out=m2cb[:, _li(c0):_li(c0) + 1])
        upcp = pP.tile([24, LP], BF16, tag="upcp")
        nc.gpsimd.memset(upcp[:], 0.0)
        nc.vector.tensor_copy(hwp(upcp[:])[:, 1:49, 1:49], hw(upc[:]))
        Y1 = pP.tile([CH, L], BF16, tag="Y1")
        m1c = pP.tile([CH, 5], FP32, tag="m1c")
        for ri, (r0, r1) in enumerate(ROW_CHUNKS):
            nr = r1 - r0
            ps = post_ps.tile([CH, 480], FP32, tag="ps")
            for tap in range(9):
                dy, dx = tap // 3, tap % 3
                rhs = hwp(upcp[:])[:, dy + r0:dy + r1, dx:dx + 48]
                nc.tensor.matmul(ps[:, :nr * 48], cvc('gwcT', tap * CH, (tap + 1) * CH),
                                 rhs, start=(tap == 0), stop=False)
            nc.tensor.matmul(ps[:, :nr * 48], cv('pw1T'), upc[:, r0 * 48:r1 * 48],
                             start=False, stop=True)
            nc.scalar.activation(Y1[:, r0 * 48:r1 * 48], ps[:, :nr * 48],
                                 F.Identity, bias=cv('gwcb'),
                                 accum_out=m1c[:, ri:ri + 1])
        Y2a = pP.tile([72, L], BF16, tag="Y2a")
        m2ca = pP.tile([72, 5], FP32, tag="m2ca")
        for ri, (c0, c1) in enumerate(MM_CHUNKS):
            ps = post_ps.tile([72, 512], FP32, tag="ps")
            nc.tensor.matmul(ps[:, :c1 - c0], cv('pw2T'), lowc[:, c0:c1], start=True, stop=True)
            nc.scalar.activation(Y2a[:, c0:c1], ps[:, :c1 - c0], F.Identity,
                                 accum_out=m2ca[:, ri:ri + 1])
        m1 = pP.tile([CH, 1], FP32, tag="m1")
        m2a_s = pP.tile([72, 1], FP32, tag="m2a_s")
        m2b_s = pP.tile([24, 1], FP32, tag="m2b_s")
        nc.vector.reduce_sum(m1[:], m1c[:], axis=mybir.AxisListType.X)
        nc.vector.reduce_sum(m2a_s[:], m2ca[:], axis=mybir.AxisListType.X)
        nc.vector.reduce_sum(m2b_s[:], m2cb[:], axis=mybir.AxisListType.X)
        smf = pP.tile([1, 2 * CH], FP32, tag="smf")
        nc.sync.dma_start(smf[0:1, 0:CH], m1[:, 0:1])
        nc.sync.dma_start(smf[0:1, CH:CH + 72], m2a_s[:, 0:1])
        nc.sync.dma_start(smf[0:1, CH + 72:2 * CH], m2b_s[:, 0:1])
        nc.vector.tensor_scalar(out=smf[:], in0=smf[:], scalar1=1.0 / L,
                                scalar2=None, op0=A.mult)
        mx = pP.tile([1, 1], FP32, tag="mx")
        nc.vector.reduce_max(mx[:], smf[:], axis=mybir.AxisListType.X)
        nc.vector.tensor_scalar(out=mx[:], in0=mx[:], scalar1=-1.0,
                                scalar2=None, op0=A.mult)
        nc.scalar.activation(smf[:], smf[:], F.Exp, bias=mx[0:1, 0:1])
        sm_s = pP.tile([1, 1], FP32, tag="sm_s")
        nc.vector.reduce_sum(sm_s[:], smf[:], axis=mybir.AxisListType.X)
        nc.vector.reciprocal(sm_s[:], sm_s[:])
        nc.vector.tensor_scalar(out=smf[:], in0=smf[:], scalar1=sm_s[0:1, 0:1],
                                scalar2=None, op0=A.mult)
        sm1 = pP.tile([CH, 1], FP32, tag="sm1")
        sm2 = pP.tile([CH, 1], FP32, tag="sm2")
        nc.sync.dma_start(sm1[:, 0:1], smf[0:1, 0:CH])
        nc.sync.dma_start(sm2[:, 0:1], smf[0:1, CH:2 * CH])
        o2f = rot.tile([CH, L], BF16, tag="pb")
        nc.sync.dma_start(o2f[0:72, :], Y2a[:])
        nc.sync.dma_start(o2f[72:96, :], lowc[:])
        o2t = pP.tile([CH, L], BF16, tag="o2t")
        yc = pP.tile([CH, L], BF16, tag="yc")
        outt = pP.tile([COUT, L], FP32, tag="outt")
        for c0, c1 in MM_CHUNKS:
            nc.vector.tensor_scalar(out=o2t[:, c0:c1], in0=o2f[:, c0:c1],
                                    scalar1=sm2[:, 0:1], scalar2=None, op0=A.mult)
            nc.vector.scalar_tensor_tensor(out=yc[:, c0:c1], in0=Y1[:, c0:c1],
                                           scalar=sm1[:, 0:1], in1=o2t[:, c0:c1],
                                           op0=A.mult, op1=A.add)
            ps = post_ps.tile([COUT, 512], FP32, tag="ps")
            nc.tensor.matmul(ps[:, :c1 - c0], cv('finT'), yc[:, c0:c1], start=True, stop=True)
            nc.scalar.activation(outt[:, c0:c1], ps[:, :c1 - c0], F.Identity, bias=cv('finb'))
            nc.sync.dma_start(out_d[:, c0:c1], outt[:, c0:c1])
        po.close()
        glob.close()
    split_multi_waits(nc, max_waits=1)
    return nc


# =============================== host side ==================================

def prep_core_inputs(inputs, b, half):
    import ml_dtypes
    f32 = np.float32
    bf16 = ml_dtypes.bfloat16
    d0 = half * HH

    def asf(a):
        return np.asarray(a, f32)

    bnscale = asf(inputs['bn_g']) / np.sqrt(np.float32(1.0 + 1e-5))
    w1 = asf(inputs['conv1_w'])[:, :, 0, 0] * bnscale[:, None]
    b1 = asf(inputs['conv1_b']) * bnscale + asf(inputs['bn_b'])

    def diag9(w, nch):
        out = np.zeros((nch, 9 * nch), f32)
        w = asf(w)
        for tap in range(9):
            dy, dx = tap // 3, tap % 3
            blk = out[:, tap * nch:(tap + 1) * nch]
            np.fill_diagonal(blk, w[:, 0, dy, dx])
        return out

    sscd = diag9(inputs['ss_conv_w'], DI)        # (192, 9*192)
    sc0 = np.zeros((128, 9 * 128), f32)
    sc1 = np.zeros((64, 9 * 64), f32)
    for tap in range(9):
        blk = sscd[:, tap * DI:(tap + 1) * DI]
        sc0[:, tap * 128:(tap + 1) * 128] = blk[0:128, 0:128]
        sc1[:, tap * 64:(tap + 1) * 64] = blk[128:192, 128:192]

    sel = np.zeros((DI, HH), f32)
    sel[np.arange(d0, d0 + HH), np.arange(HH)] = 1.0

    xp = asf(inputs['ss_xproj_w'])               # (4, 38, 192)
    xpTa = np.zeros((128, K4 * 38), f32)
    xpTb = np.zeros((64, K4 * 38), f32)
    for k in range(K4):
        xpT = xp[k].T                            # (192, 38)
        xpTa[:, k * 38:(k + 1) * 38] = xpT[0:128]
        xpTb[:, k * 38:(k + 1) * 38] = xpT[128:192]

    dtw = asf(inputs['ss_dt_w'])
    dtwT = np.zeros((R, K4 * HH), f32)
    for k in range(K4):
        dtwT[:, k * HH:(k + 1) * HH] = dtw[k][d0:d0 + HH, :].T

    dtb_full = asf(inputs['ss_dt_b'])
    Alog = asf(inputs['ss_Alog']).reshape(K4, DI, N)
    Dv = asf(inputs['ss_D']).reshape(K4, DI)
    dtb_p = np.zeros((128, NT), f32)
    Ap = np.zeros((128, NT * N), f32)
    for (j, o0, o1, k, dd0, dd1) in SECTIONS:
        dtb_p[o0:o1, j] = dtb_full[k, d0 + dd0:d0 + dd1]
        Ap[o0:o1, j * N:(j + 1) * N] = -np.exp(Alog[k, d0 + dd0:d0 + dd1])
    Dsum = Dv[:, d0:d0 + HH].sum(0)[:, None]

    gw = asf(inputs['gwc_w'])
    gT = np.zeros((24, 9 * CH), f32)
    for tap in range(9):
        dy, dx = tap // 3, tap % 3
        blk = np.zeros((24, CH), f32)
        blk[0:12, 0:48] = gw[0:48, :, dy, dx].T
        blk[12:24, 48:96] = gw[48:96, :, dy, dx].T
        gT[:, tap * CH:(tap + 1) * CH] = blk

    owT = asf(inputs['ss_out_w']).T              # (192, 96)
    outn_g = asf(inputs['ss_outn_g'])
    outn_b = asf(inputs['ss_outn_b'])

    vals32 = {
        'b1': b1[:, None],
        'linb': asf(inputs['lin_b'])[:, None],
        'dw1b': asf(inputs['dw1_b'])[:, None],
        'dw2b': asf(inputs['dw2_b'])[:, None],
        'scb0': asf(inputs['ss_conv_b'])[0:128, None],
        'scb1': asf(inputs['ss_conv_b'])[128:192, None],
        'dtb': dtb_p, 'Ap': Ap, 'Dsum': Dsum,
        'outngA': outn_g[0:96, None], 'outngB': outn_g[96:192, None],
        'outnbA': outn_b[0:96, None], 'outnbB': outn_b[96:192, None],
        'ag1b': asf(inputs['ag1_b'])[:, None],
        'ag2b': asf(inputs['ag2_b'])[:, None],
        'lng': asf(inputs['ln_g'])[:, None],
        'lnb': asf(inputs['ln_b'])[:, None],
        'gwcb': asf(inputs['gwc_b'])[:, None],
        'finb': asf(inputs['fin_b'])[:, None],
    }
    valsbf = {
        'w1T': w1.T,
        'linT': asf(inputs['lin_w']).T,
        'dw1dg': diag9(inputs['dw1_w'], CH),
        'dw2dg': diag9(inputs['dw2_w'], CH),
        'inwT': asf(inputs['ss_in_w']).T,        # (96, 384) full z
        'sc0dg': sc0, 'sc1dg': sc1,
        'sel0': sel[0:128], 'sel1': sel[128:192],
        'xpTa': xpTa, 'xpTb': xpTb,
        'dtwT': dtwT,
        'ident': np.eye(128, dtype=f32),
        'outwTa': owT[0:96], 'outwTb': owT[96:192],
        'ag1T': asf(inputs['ag1_w'])[:, :, 0, 0].T,
        'ag2T': asf(inputs['ag2_w'])[:, :, 0, 0].T,
        'sq1T': asf(inputs['sq1_w'])[:, :, 0, 0].T,
        'sq2T': asf(inputs['sq2_w'])[:, :, 0, 0].T,
        'gwcT': gT,
        'pw1T': asf(inputs['pwc1_w'])[:, :, 0, 0].T,
        'pw2T': asf(inputs['pwc2_w'])[:, :, 0, 0].T,
        'finT': asf(inputs['fin_w']).T,
        'bcv': np.concatenate([np.full((1, 96), 1.0 / DI, f32),
                               np.full((1, 96), 1.0 / CH, f32),
                               np.ones((1, 96), f32)], axis=1),
    }

    blob32 = np.zeros((128, W32), f32)
    for nm, p, c in CONSTS_F32:
        o = OFF32[nm][0]
        v = vals32[nm]
        assert v.shape == (p, c), (nm, v.shape, (p, c))
        blob32[0:p, o:o + c] = v
    blobbf = np.zeros((128, WBF), bf16)
    for nm, p, c in CONSTS_BF16:
        o = OFFBF[nm][0]
        v = valsbf[nm]
        assert v.shape == (p, c), (nm, v.shape, (p, c))
        blobbf[0:p, o:o + c] = v.astype(bf16)

    return {
        'x': np.ascontiguousarray(asf(inputs['x'])[b].reshape(CIN, L).astype(bf16)),
        'c32': blob32,
        'cbf': np.ascontiguousarray(blobbf),
    }


_NC_CACHE = {}


def get_nc():
    if 'nc' not in _NC_CACHE:
        _NC_CACHE['nc'] = build_nc()
    return _NC_CACHE['nc']


def kernel(**inputs):
    from concourse.bass_utils import run_bass_kernel_spmd
    nc = get_nc()
    in_maps = [prep_core_inputs(inputs, c // 2, c % 2) for c in range(8)]
    res = run_bass_kernel_spmd(nc, in_maps, core_ids=list(range(8)))
    out = np.zeros((B_, COUT, H, W), np.float32)
    for b in range(B_):
        out[b] = res.results[2 * b]['out'].reshape(COUT, H, W)
    return out


# revision 63
# speedup vs baseline: 1.0721x; 1.0097x over previous
"""Self-contained Trainium2 Bass kernel for the CR-VSS block (8 cores)."""

# ---- TileContext drain-wait patch (walrus 1-wait limit) ----
"""Patch TileContext._drain_and_barrier: the axon-client walrus rejects
instructions carrying >2 sem waits ("Too many sync wait commands" in
setupSyncWait for CTRL structs). Redistribute the exit-drain's waits across
preceding SP nop instructions, each carrying at most MAX_WAITS."""
from concourse.tile import TileContext, ScopedClock

MAX_WAITS = 1


def _patched_drain_and_barrier(self, tick_clock, wait_clock):
    nc = self.nc
    drain_inst = nc.sync.drain()
    wait_clock.add_sem_waits(
        drain_inst.ins, ScopedClock({None: tick_clock.global_clock})
    )

    waits = list(drain_inst.ins.sync_info.on_wait or [])
    if len(waits) > MAX_WAITS:
        bb = nc.cur_bb.bb
        assert bb.instructions[-1] is drain_inst.ins
        # strip waits from the drain, re-emit them on nop carriers
        drain_inst.ins.sync_info.on_wait = waits[:0]
        carriers = []
        import concourse.mybir as mybir
        for i in range(0, len(waits), MAX_WAITS):
            nop = nc.sync.nop(nofuse=True)
            nop.ins.sync_info = mybir.SyncInfo(
                on_wait=waits[i:i + MAX_WAITS], on_update=[]
            )
            carriers.append(nop.ins)
        # move carriers before the drain
        insts = list(bb.instructions)
        assert insts[-len(carriers) - 1] is drain_inst.ins
        reordered = insts[:-len(carriers) - 1] + insts[-len(carriers):] + [drain_inst.ins]
        while len(bb.instructions):
            bb.instructions.pop()
        for x in reordered:
            bb.instructions.append(x)

    nc.all_engine_barrier()
    assert self.sems is not None
    popped = nc._tile_sem_poison_stack.pop()
    assert popped is self._sem_poison
    nc.clear_and_free_semaphores(list(self.sems.allocated().values()))
    nc.all_engine_barrier()


def apply():
    TileContext._drain_and_barrier = _patched_drain_and_barrier


def split_multi_waits(nc, max_waits=1):
    """Post-pass: walrus CTRL codegen rejects instructions with more than
    one sem wait. Move extra waits onto same-engine NoOp carriers."""
    import concourse.mybir as mybir
    for f in nc.m.functions:
        for bb in f.blocks:
            insts = list(bb.instructions)
            out = []
            changed = False
            for ins in insts:
                si = ins.sync_info
                if si is not None and si.on_wait and len(si.on_wait) > max_waits:
                    waits = list(si.on_wait)
                    for i, w in enumerate(waits[max_waits:]):
                        nop = mybir.InstNoOp.__new__(
                            mybir.InstNoOp, name=f"{ins.name}-xw{i}", ins=[], outs=[])
                        nop.engine = ins.engine
                        nop.sync_info = mybir.SyncInfo(on_wait=[w], on_update=[])
                        out.append(nop)
                    ins.sync_info = mybir.SyncInfo(
                        on_wait=waits[:max_waits],
                        on_update=list(si.on_update or []))
                    changed = True
                out.append(ins)
            if changed:
                while len(bb.instructions):
                    bb.instructions.pop()
                for x in out:
                    bb.instructions.append(x)

apply()

# ---- kernel ----
"""Trainium2 Bass kernel for nn_CR_VSS (VSS block with SS2D selective scan).

Sharding: 8 cores = 4 samples x 2 d_inner-halves. Each core runs the full
pre-stage for its sample, scans its 96-channel d-half across all 4
cross-scan directions (packed into 3x128-partition tiles), then the pair
exchanges y-halves with ONE AllGather; LN + out-proj + post-stage run
locally (z is computed full-width in the in-proj so no second collective).

Scan: h_t = exp(A*delta_t)*h_{t-1} + delta_t*u_t*B_t per (k,d,n) via
tensor_tensor_scan; n in groups of 4 with batched B/C partition-broadcast
DMAs (double-buffered); y accumulated over n with identity-lhsT PSUM
matmuls, merged into pixel-order ysum straight from PSUM per t-chunk.
"""
import numpy as np
from contextlib import ExitStack

import concourse.bass as bass
import concourse.mybir as mybir

F = mybir.ActivationFunctionType
A = mybir.AluOpType
FP32 = mybir.dt.float32
BF16 = mybir.dt.bfloat16

B_, CIN, CH, COUT, H, W = 4, 96, 96, 96, 48, 48
DI, N, R, K4 = 192, 16, 6, 4
L = H * W               # 2304
HH = 96                 # d-half per core
NT = 3                  # packed (k,d) tiles: 4*96 = 384 = 3*128
HP = 50
LP = 2500
TC = 768                # scan t-chunk (16 rows of 48)
TCH = [(0, 768), (768, 1536), (1536, 2304)]
NG = 4                  # scan n-group (broadcast batch)

# packed (k,d) rows -> (tile j, offset): sections (j, o0, o1, k, d0, d1).
# Section offsets are all 0/32/64 so PE matmuls can write them directly.
SECTIONS = [
    (0, 0, 32, 1, 0, 32),
    (0, 32, 128, 0, 0, 96),
    (1, 0, 64, 1, 32, 96),
    (1, 64, 128, 2, 0, 64),
    (2, 0, 32, 2, 64, 96),
    (2, 32, 128, 3, 0, 96),
]

MM_CHUNKS = [(0, 512), (512, 1024), (1024, 1536), (1536, 2048), (2048, 2304)]
ROW_CHUNKS = [(0, 10), (10, 20), (20, 30), (30, 40), (40, 48)]
SUBS768 = [(0, 512), (512, 768)]
INW_BLOCKS = [(0, 128), (128, 256), (256, 384)]

REPLICA_GROUPS = [[0, 1], [2, 3], [4, 5], [6, 7]]

# ---- const blobs (shared layout between host packing and kernel views) ----
CONSTS_F32 = [
    ('b1', 96, 1), ('linb', 96, 1),
    ('dw1b', 96, 1), ('dw2b', 96, 1),
    ('scb0', 128, 1), ('scb1', 64, 1),
    ('dtb', 128, 3), ('Ap', 128, 48), ('Dsum', 96, 1),
    ('outngA', 96, 1), ('outngB', 96, 1), ('outnbA', 96, 1), ('outnbB', 96, 1),
    ('ag1b', 48, 1), ('ag2b', 96, 1), ('lng', 96, 1), ('lnb', 96, 1),
    ('gwcb', 96, 1), ('finb', 96, 1),
]
CONSTS_BF16 = [
    ('w1T', 96, 96), ('linT', 96, 96),
    ('dw1dg', 96, 864), ('dw2dg', 96, 864),
    ('inwT', 96, 384),
    ('sc0dg', 128, 1152), ('sc1dg', 64, 576),
    ('sel0', 128, 96), ('sel1', 64, 96),
    ('xpTa', 128, 152), ('xpTb', 64, 152),
    ('dtwT', 6, 384),
    ('ident', 128, 128),
    ('outwTa', 96, 96), ('outwTb', 96, 96),
    ('ag1T', 96, 48), ('ag2T', 48, 96),
    ('sq1T', 48, 24), ('sq2T', 48, 24),
    ('gwcT', 24, 864), ('pw1T', 24, 96), ('pw2T', 24, 72),
    ('finT', 96, 96), ('bcv', 1, 288),
]

OFF32 = {}
_o = 0
for _nm, _p, _c in CONSTS_F32:
    OFF32[_nm] = (_o, _p, _c)
    _o += _c
W32 = _o
OFFBF = {}
_o = 0
for _nm, _p, _c in CONSTS_BF16:
    OFFBF[_nm] = (_o, _p, _c)
    _o += _c
WBF = _o


def build_nc():
    nc = bass.Bass(trn_type="TRN2", num_devices=8)

    x_d = nc.dram_tensor("x", [CIN, L], BF16, kind="ExternalInput")
    c32_d = nc.dram_tensor("c32", [128, W32], FP32, kind="ExternalInput")
    cbf_d = nc.dram_tensor("cbf", [128, WBF], BF16, kind="ExternalInput")
    out_d = nc.dram_tensor("out", [COUT, L], FP32, kind="ExternalOutput")

    B_dram = nc.dram_tensor("B_dram", [K4 * N, L], BF16)
    C_dram = nc.dram_tensor("C_dram", [K4 * N, L], BF16)
    y_dram = nc.dram_tensor("y_dram", [HH, L], BF16)
    yg_dram = nc.dram_tensor("yg_dram", [DI, L], BF16)
    st_dram = nc.dram_tensor("st_dram", [2, L], BF16)

    def hw(ap):
        return ap.rearrange("p (h w) -> p h w", h=H)

    def hwp(ap):
        return ap.rearrange("p (h w) -> p h w", h=HP)

    def whv(ap):
        return ap.rearrange("p (h w) -> p w h", h=H)

    with TileContext(nc) as tc:
        glob = ExitStack()
        cst = glob.enter_context(tc.tile_pool(name="cst", bufs=1))
        lngA = glob.enter_context(tc.tile_pool(name="lngA", bufs=1))

        cst32 = cst.tile([128, W32], FP32, tag="cst32")
        cstbf = cst.tile([128, WBF], BF16, tag="cstbf")
        nc.sync.dma_start(cst32[:], c32_d[:])
        nc.sync.dma_start(cstbf[:], cbf_d[:])

        def cvc(nm, a0=0, a1=None, p0=0, p1=None):
            d, tile = (OFF32, cst32) if nm in OFF32 else (OFFBF, cstbf)
            o, p, c = d[nm]
            if a1 is None:
                a1 = c
            if p1 is None:
                p1 = p
            return tile[p0:p1, o + a0:o + a1]

        cv = cvc

        ones96 = cst.tile([HH, 1], BF16, tag="ones96")
        nc.vector.memset(ones96[:], 1.0)

        # long-lived across phases
        z0 = lngA.tile([128, L], BF16, tag="z0")     # z rows 0:128
        z1 = lngA.tile([64, L], BF16, tag="z1")      # z rows 128:192
        x2 = lngA.tile([CH, L], BF16, tag="x2")
        lngB = ExitStack()
        lngB_p = lngB.enter_context(tc.tile_pool(name="lngB_p", bufs=1))
        xch = lngB_p.tile([HH, L], BF16, tag="xch")
        dp = [lngB_p.tile([128, L], BF16, tag=f"dp{j}", name=f"dp{j}") for j in range(NT)]
        # xsp holds packed scan-order xs, overwritten in place with delta*u
        xsp = [lngB_p.tile([128, L], BF16, tag=f"xsp{j}", name=f"xsp{j}") for j in range(NT)]
        ysum = lngB_p.tile([HH, L], BF16, tag="ysum")

        # ================= pre-stage =================
        pre = ExitStack()
        pre_ps = pre.enter_context(tc.tile_pool(name="pre_ps", bufs=6, space="PSUM"))
        pA = pre.enter_context(tc.tile_pool(name="pA", bufs=1))
        pB = pre.enter_context(tc.tile_pool(name="pB", bufs=1))

        xt = pA.tile([CIN, L], BF16, tag="xt")
        nc.sync.dma_start(xt[:], x_d[:])

        # conv1x1 (+folded BN) + ReLU
        h1 = pA.tile([CH, L], BF16, tag="h1")
        for c0, c1 in MM_CHUNKS:
            ps = pre_ps.tile([CH, 512], FP32, tag="ps")
            nc.tensor.matmul(ps[:, :c1 - c0], cv('w1T'), xt[:, c0:c1], start=True, stop=True)
            nc.scalar.activation(h1[:, c0:c1], ps[:, :c1 - c0], F.Relu, bias=cv('b1'))
        # token linear
        h2 = pA.tile([CH, L], BF16, tag="h2")
        for c0, c1 in MM_CHUNKS:
            ps = pre_ps.tile([CH, 512], FP32, tag="ps")
            nc.tensor.matmul(ps[:, :c1 - c0], cv('linT'), h1[:, c0:c1], start=True, stop=True)
            nc.vector.tensor_scalar(out=h2[:, c0:c1], in0=ps[:, :c1 - c0],
                                    scalar1=cv('linb'), scalar2=None, op0=A.add)
        h2p = pA.tile([CH, LP], BF16, tag="h2p")
        nc.gpsimd.memset(h2p[:], 0.0)
        for (r0, r1) in ROW_CHUNKS:
            nc.vector.tensor_copy(hwp(h2p[:])[:, r0 + 1:r1 + 1, 1:49],
                                  hw(h2[:])[:, r0:r1, :])

        def dwconv(dst, src_p, dgname, biasname, nch):
            for (r0, r1) in ROW_CHUNKS:
                nr = r1 - r0
                ps = pre_ps.tile([128, 480], FP32, tag="ps")
                for tap in range(9):
                    dy, dx = tap // 3, tap % 3
                    rhs = hwp(src_p[:])[:, dy + r0:dy + r1, dx:dx + 48]
                    nc.tensor.matmul(ps[:nch, :nr * 48],
                                     cvc(dgname, tap * nch, (tap + 1) * nch),
                                     rhs, start=(tap == 0), stop=(tap == 8))
                nc.scalar.activation(dst[:, r0 * 48:r1 * 48], ps[:nch, :nr * 48],
                                     F.Silu, bias=cv(biasname))

        x1 = pB.tile([CH, L], BF16, tag="x1")
        dwconv(x1, h2p, 'dw1dg', 'dw1b', CH)

        # in-proj: xi (192) + FULL z (192)
        xi0 = pB.tile([128, L], BF16, tag="xi0")
        xi1 = pB.tile([64, L], BF16, tag="xi1")
        for mb, (m0, m1) in enumerate(INW_BLOCKS):
            for c0, c1 in MM_CHUNKS:
                ps = pre_ps.tile([128, 512], FP32, tag="ps")
                nc.tensor.matmul(ps[:m1 - m0, :c1 - c0], cvc('inwT', m0, m1),
                                 x1[:, c0:c1], start=True, stop=True)
                if mb == 0:
                    nc.vector.tensor_copy(xi0[:, c0:c1], ps[:128, :c1 - c0])
                elif mb == 1:
                    nc.scalar.copy(xi1[:, c0:c1], ps[0:64, :c1 - c0])
                    nc.scalar.copy(z0[0:64, c0:c1], ps[64:128, :c1 - c0])
                else:
                    nc.scalar.copy(z0[64:128, c0:c1], ps[0:64, :c1 - c0])
                    nc.scalar.copy(z1[0:64, c0:c1], ps[64:128, :c1 - c0])

        xi0p = pB.tile([128, LP], BF16, tag="xi0p")
        xi1p = pB.tile([64, LP], BF16, tag="xi1p")
        nc.gpsimd.memset(xi0p[:], 0.0)
        nc.gpsimd.memset(xi1p[:], 0.0)
        for (r0, r1) in ROW_CHUNKS:
            nc.vector.tensor_copy(hwp(xi0p[:])[:, r0 + 1:r1 + 1, 1:49],
                                  hw(xi0[:])[:, r0:r1, :])
            nc.vector.tensor_copy(hwp(xi1p[:])[:, r0 + 1:r1 + 1, 1:49],
                                  hw(xi1[:])[:, r0:r1, :])
        xc0 = pB.tile([128, L], BF16, tag="xc0")
        xc1 = pB.tile([64, L], BF16, tag="xc1")
        dwconv(xc0, xi0p, 'sc0dg', 'scb0', 128)
        dwconv(xc1, xi1p, 'sc1dg', 'scb1', 64)

        # d-half extraction + wh copy
        for c0, c1 in MM_CHUNKS:
            ps = pre_ps.tile([HH, 512], FP32, tag="ps")
            nc.tensor.matmul(ps[:, :c1 - c0], cv('sel0'), xc0[:, c0:c1], start=True, stop=False)
            nc.tensor.matmul(ps[:, :c1 - c0], cv('sel1'), xc1[:, c0:c1], start=False, stop=True)
            nc.vector.tensor_copy(xch[:, c0:c1], ps[:, :c1 - c0])
        xwhh = pB.tile([HH, L], BF16, tag="xwhh")
        for (t0, t1) in TCH:
            w0, w1 = t0 // 48, t1 // 48
            nc.vector.tensor_copy(hw(xwhh[:])[:, w0:w1, :],
                                  whv(xch[:])[:, w0:w1, :])

        # xproj (compact 38 rows: 0:6 dts, 6:22 B, 22:38 C) in scan order
        def xc_read(k, c0, c1):
            if k == 0:
                return (xc0[:, c0:c1], xc1[:, c0:c1])
            if k == 1:
                return (whv(xc0[:])[:, c0 // 48:c1 // 48, :],
                        whv(xc1[:])[:, c0 // 48:c1 // 48, :])
            if k == 2:
                return (xc0[:, L - c1:L - c0][:, ::-1],
                        xc1[:, L - c1:L - c0][:, ::-1])
            r0 = whv(xc0[:])[:, (L - c1) // 48:(L - c0) // 48, :][:, ::-1, ::-1]
            r1 = whv(xc1[:])[:, (L - c1) // 48:(L - c0) // 48, :][:, ::-1, ::-1]
            return (r0, r1)

        # row-chunk outer so all 4 directions' early columns finish first;
        # B/C are written to DRAM per scan chunk so ci=0 broadcasts can
        # start while xproj still works on later chunks.
        stage = [pB.tile([38, L], BF16, tag=f"stg{k}", name=f"stg{k}") for k in range(K4)]
        done_w = 0
        for ri, (rr0, rr1) in enumerate(ROW_CHUNKS):
            c0, c1 = rr0 * 48, rr1 * 48
            nf = c1 - c0
            for k in range(K4):
                ra, rb = xc_read(k, c0, c1)
                ps = pre_ps.tile([38, 480], FP32, tag="ps")
                nc.tensor.matmul(ps[:, :nf], cvc('xpTa', k * 38, (k + 1) * 38), ra,
                                 start=True, stop=False)
                nc.tensor.matmul(ps[:, :nf], cvc('xpTb', k * 38, (k + 1) * 38), rb,
                                 start=False, stop=True)
                nc.vector.tensor_copy(stage[k][:, c0:c1], ps[:, :nf])
            while done_w < len(TCH) and TCH[done_w][1] <= c1:
                t0, t1 = TCH[done_w]
                for k in range(K4):
                    nc.sync.dma_start(B_dram[k * N:(k + 1) * N, t0:t1],
                                      stage[k][6:22, t0:t1])
                    nc.sync.dma_start(C_dram[k * N:(k + 1) * N, t0:t1],
                                      stage[k][22:38, t0:t1])
                done_w += 1

        # delta: packed matmuls then softplus on full 128-partition tiles
        def mm_windows(a0, a1):
            if a0 == 0:
                return [(0, a1)]
            res = []
            x = a0
            while x < a1:
                if x % 64 == 32:
                    e = min(a1, x + 32)
                else:  # x == 64
                    e = min(a1, 128)
                res.append((x, e))
                x = e
            return res

        for (cc0, cc1) in MM_CHUNKS:
            cw = cc1 - cc0
            for j in range(NT):
                ex = pre_ps.tile([128, 512], FP32, tag="ps")
                for (jj, o0, o1, k, d0, d1) in SECTIONS:
                    if jj != j:
                        continue
                    for (w0, w1) in mm_windows(o0, o1):
                        dd0 = d0 + (w0 - o0)
                        dd1 = d0 + (w1 - o0)
                        nc.tensor.matmul(ex[w0:w1, :cw],
                                         cvc('dtwT', k * 96 + dd0, k * 96 + dd1),
                                         stage[k][0:6, cc0:cc1], start=True, stop=True)
                # softplus(x+b) = ln(1 + exp(x+b)) (no softplus act table on HW)
                ex2 = pre_ps.tile([128, 512], FP32, tag="ps")
                nc.scalar.activation(ex2[:, :cw], ex[:, :cw], F.Exp,
                                     bias=cvc('dtb', j, j + 1))
                nc.scalar.activation(dp[j][:, cc0:cc1], ex2[:, :cw], F.Ln, bias=1.0)

        # pack scan-order xs (Act copies handle partition shift + flips),
        # then overwrite in place with delta*u = dp*xs.
        # Act partition windows must not cross engine block boundaries on
        # EITHER side: allowed starts 0/32/64/96; a start-32 window may not
        # cross 64. split2 chops a shifted copy accordingly.
        def _legal_span(s):
            return 32 if s == 32 else 128 - s if s else 128

        def split2(o0, i0, ln):
            res = []
            x = 0
            while x < ln:
                step = min(ln - x, _legal_span(o0 + x), _legal_span(i0 + x))
                res.append((x, x + step))
                x += step
            return res

        for (t0, t1) in TCH:
            for (j, o0, o1, k, d0, d1) in SECTIONS:
                v = xwhh if k in (1, 3) else xch
                if k < 2:
                    # forward sections: contiguous rows, cheap DMA shift
                    nc.sync.dma_start(xsp[j][o0:o1, t0:t1], v[d0:d1, t0:t1])
                    continue
                for (w0, w1) in split2(o0, d0, o1 - o0):
                    nc.scalar.copy(xsp[j][o0 + w0:o0 + w1, t0:t1],
                                   v[d0 + w0:d0 + w1, ::-1][:, t0:t1])
            for j in range(NT):
                nc.vector.tensor_mul(xsp[j][:, t0:t1], dp[j][:, t0:t1],
                                     xsp[j][:, t0:t1])

        # branch-2 dwconv last: x2 is only needed in the post stage, so
        # this fills PE/Act slack once the scan inputs are queued
        dwconv(x2, h2p, 'dw2dg', 'dw2b', CH)

        pre.close()

        # ================= scan =================
        sc = ExitStack()
        bbp = sc.enter_context(tc.tile_pool(name="bbp", bufs=2))
        spool = sc.enter_context(tc.tile_pool(name="spool", bufs=2))
        scan_ps = sc.enter_context(tc.tile_pool(name="scan_ps", bufs=1, space="PSUM"))
        stp = sc.enter_context(tc.tile_pool(name="stp", bufs=1))
        state = [stp.tile([128, N], FP32, tag=f"st{j}", name=f"state{j}")
                 for j in range(NT)]
        yd = [stp.tile([HH, L], BF16, tag=f"yd{k}", name=f"yd{k}") for k in range(K4)]

        pending_drain = None
        for ci, (c0, c1) in enumerate(TCH):
            ypsum = [scan_ps.tile([128, TC], FP32, tag=f"yps{j}", name=f"yps{j}_{ci}")
                     for j in range(NT)]
            for g in range(K4):
                Bb = [bbp.tile([128, NG * TC], BF16, tag=f"Bb{j}", name=f"Bb{j}_{ci}_{g}")
                      for j in range(NT)]
                Cb = [bbp.tile([128, NG * TC], BF16, tag=f"Cb{j}", name=f"Cb{j}_{ci}_{g}")
                      for j in range(NT)]
                for (j, o0, o1, k, d0, d1) in SECTIONS:
                    nc.sync.dma_start(
                        Bb[j][o0:o1, :],
                        B_dram[k * N + NG * g:k * N + NG * g + NG, c0:c1]
                        .partition_broadcast(o1 - o0))
                    nc.scalar.dma_start(
                        Cb[j][o0:o1, :],
                        C_dram[k * N + NG * g:k * N + NG * g + NG, c0:c1]
                        .partition_broadcast(o1 - o0))
                if pending_drain is not None:
                    pending_drain()
                    pending_drain = None
                for n4 in range(NG):
                    n = NG * g + n4
                    for j in range(NT):
                        at = spool.tile([128, TC], BF16, tag=f"at{j}", name=f"at{j}")
                        nc.scalar.activation(at[:], dp[j][:, c0:c1], F.Exp,
                                             scale=cvc('Ap', j * N + n, j * N + n + 1))
                        bt = spool.tile([128, TC], BF16, tag=f"bt{j}", name=f"bt{j}")
                        nc.vector.tensor_mul(bt[:], xsp[j][:, c0:c1],
                                             Bb[j][:, n4 * TC:(n4 + 1) * TC])
                        ht = spool.tile([128, TC], BF16, tag=f"ht{j}", name=f"ht{j}")
                        if ci > 0:
                            # fold carry state into bt[0] so the scan can use
                            # the cheap zero-init form
                            nc.vector.scalar_tensor_tensor(
                                out=bt[:, 0:1], in0=at[:, 0:1],
                                scalar=state[j][:, n:n + 1], in1=bt[:, 0:1],
                                op0=A.mult, op1=A.add)
                        nc.vector.tensor_tensor_scan(ht[:], at[:], bt[:], 0.0,
                                                     A.mult, A.add)
                        if ci < 2:
                            nc.vector.tensor_copy(state[j][:, n:n + 1], ht[:, TC - 1:TC])
                        gt = spool.tile([128, TC], BF16, tag=f"gt{j}", name=f"gt{j}")
                        nc.vector.tensor_mul(gt[:], ht[:],
                                             Cb[j][:, n4 * TC:(n4 + 1) * TC])
                        for (s0, s1) in SUBS768:
                            nc.tensor.matmul(ypsum[j][:, s0:s1], cv('ident'),
                                             gt[:, s0:s1],
                                             start=(n == 0), stop=(n == N - 1))
            # drain this chunk's PSUM into per-direction scan-order tiles
            # (Act copies allow the partition shift). Deferred past the next
            # chunk's broadcast issue so the boundary doesn't stall Act.
            def _drain(yps=ypsum, cc0=c0, cc1=c1):
                for (j, o0, o1, k, d0, d1) in SECTIONS:
                    for (w0, w1) in split2(d0, o0, d1 - d0):
                        nc.scalar.copy(yd[k][d0 + w0:d0 + w1, cc0:cc1],
                                       yps[j][o0 + w0:o0 + w1, :])
            pending_drain = _drain
        if pending_drain is not None:
            pending_drain()
            pending_drain = None
        # merge directions into pixel order + D*u term
        tmp96 = stp.tile([HH, L], BF16, tag="tmp96")
        nc.vector.tensor_add(ysum[:], yd[0][:], yd[2][:, ::-1])
        nc.vector.tensor_add(tmp96[:], yd[1][:], yd[3][:, ::-1])
        nc.vector.tensor_add(ysum[:], ysum[:], whv(tmp96[:]))
        nc.vector.scalar_tensor_tensor(out=ysum[:], in0=xch[:], scalar=cv('Dsum'),
                                       in1=ysum[:], op0=A.mult, op1=A.add)
        nc.sync.dma_start(y_dram[:], ysum[:])
        nc.gpsimd.collective_compute(
            "AllGather", A.bypass, replica_groups=REPLICA_GROUPS,
            ins=[y_dram[:]], outs=[yg_dram[:]])
        sc.close()
        lngB.close()

        # ================= post =================
        po = ExitStack()
        post_ps = po.enter_context(tc.tile_pool(name="post_ps", bufs=6, space="PSUM"))
        pP = po.enter_context(tc.tile_pool(name="pP", bufs=1))
        rot = po.enter_context(tc.tile_pool(name="rot", bufs=4))

        # branch 2 + silu(z): no dep on the collective, runs under it
        g1 = rot.tile([48, L], BF16, tag="pb")
        for c0, c1 in MM_CHUNKS:
            ps = post_ps.tile([48, 512], FP32, tag="ps")
            nc.tensor.matmul(ps[:, :c1 - c0], cv('ag1T'), x2[:, c0:c1], start=True, stop=True)
            nc.scalar.activation(g1[:, c0:c1], ps[:, :c1 - c0], F.Relu, bias=cv('ag1b'))
        gat = rot.tile([CH, L], BF16, tag="pb")
        for c0, c1 in MM_CHUNKS:
            ps = post_ps.tile([CH, 512], FP32, tag="ps")
            nc.tensor.matmul(ps[:, :c1 - c0], cv('ag2T'), g1[:, c0:c1], start=True, stop=True)
            nc.scalar.activation(gat[:, c0:c1], ps[:, :c1 - c0], F.Sigmoid, bias=cv('ag2b'))
        x2g = pP.tile([CH, L], BF16, tag="x2g")
        nc.vector.tensor_mul(x2g[:], x2[:], gat[:])

        zsA = pP.tile([HH, L], BF16, tag="zsA")
        zsB = pP.tile([HH, L], BF16, tag="zsB")
        nc.scalar.activation(zsA[:], z0[0:96, :], F.Silu)
        nc.scalar.activation(zsB[0:32, :], z0[96:128, :], F.Silu)
        nc.scalar.activation(zsB[32:64, :], z1[0:32, :], F.Silu)
        nc.scalar.activation(zsB[64:96, :], z1[32:64, :], F.Silu)

        ygA = pP.tile([HH, L], BF16, tag="ygA")
        ygB = pP.tile([HH, L], BF16, tag="ygB")
        nc.sync.dma_start(ygA[:], yg_dram[0:HH, :])
        nc.sync.dma_start(ygB[:], yg_dram[HH:DI, :])

        sA = pP.tile([1, L], BF16, tag="sA")
        sB = pP.tile([1, L], BF16, tag="sB")
        sM = pP.tile([1, L], BF16, tag="sM")

        def ln_stats(cinv):
            # in: sA=raw sum, sB=raw sumsq; leaves rstd in sB (sA stays raw sum)
            with nc.allow_low_precision(reason="LN stats kept bf16; rel-err verified"):
                nc.scalar.activation(sM[:], sA[:], F.Square, scale=cinv)
                nc.vector.scalar_tensor_tensor(out=sB[:], in0=sB[:], scalar=cinv,
                                               in1=sM[:], op0=A.mult, op1=A.subtract)
                nc.vector.tensor_scalar(out=sB[:], in0=sB[:], scalar1=1e-5,
                                        scalar2=None, op0=A.add)
                nc.vector.reciprocal(sB[:], sB[:])
                nc.scalar.activation(sB[:], sB[:], F.Sqrt)

        # LayerNorm over full DI (local stats via ones-matmul), fully
        # chunk-pipelined behind the per-chunk yg reads
        ysqA = rot.tile([HH, L], BF16, tag="pb")
        ysqB = rot.tile([HH, L], BF16, tag="pb")
        for c, (c0, c1) in enumerate(MM_CHUNKS):
            nc.vector.tensor_mul(ysqA[:, c0:c1], ygA[:, c0:c1], ygA[:, c0:c1])
            nc.vector.tensor_mul(ysqB[:, c0:c1], ygB[:, c0:c1], ygB[:, c0:c1])
            ps = post_ps.tile([1, 512], FP32, tag="ps")
            nc.tensor.matmul(ps[:, :c1 - c0], ones96[:], ygA[:, c0:c1], start=True, stop=False)
            nc.tensor.matmul(ps[:, :c1 - c0], ones96[:], ygB[:, c0:c1], start=False, stop=True)
            nc.scalar.copy(sA[0:1, c0:c1], ps[:, :c1 - c0])
            ps2 = post_ps.tile([1, 512], FP32, tag="ps")
            nc.tensor.matmul(ps2[:, :c1 - c0], ones96[:], ysqA[:, c0:c1], start=True, stop=False)
            nc.tensor.matmul(ps2[:, :c1 - c0], ones96[:], ysqB[:, c0:c1], start=False, stop=True)
            nc.scalar.copy(sB[0:1, c0:c1], ps2[:, :c1 - c0])

        ln_stats(1.0 / DI)

        def apply_ln(pairs, bco):
            # pairs: list of (dst, src, gname, bname); bco: bcv column offset
            # holding 1/DI or 1/CH (folds the mean division into the
            # broadcast lhsT)
            for c0, c1 in MM_CHUNKS:
                cw = c1 - c0
                psm = post_ps.tile([HH, 512], FP32, tag="ps")
                nc.tensor.matmul(psm[:, :cw], cvc('bcv', bco, bco + HH),
                                 sA[:, c0:c1], start=True, stop=True)
                psr = post_ps.tile([HH, 512], FP32, tag="ps")
                nc.tensor.matmul(psr[:, :cw], cvc('bcv', 192, 192 + HH),
                                 sB[:, c0:c1], start=True, stop=True)
                for (dst, srct, gname, bname) in pairs:
                    nc.vector.tensor_sub(dst[:, c0:c1], srct[:, c0:c1],
                                         psm[:, :cw])
                    nc.vector.tensor_mul(dst[:, c0:c1], dst[:, c0:c1],
                                         psr[:, :cw])
                    nc.vector.tensor_scalar(out=dst[:, c0:c1], in0=dst[:, c0:c1],
                                            scalar1=cv(gname), scalar2=cv(bname),
                                            op0=A.mult, op1=A.add)

        ynA = rot.tile([HH, L], BF16, tag="pb")
        ynB = rot.tile([HH, L], BF16, tag="pb")
        apply_ln([(ynA, ygA, 'outngA', 'outnbA'),
                  (ynB, ygB, 'outngB', 'outnbB')], 0)

        # fused chunk pipeline: ygz -> out-proj -> yb -> ybsq -> LN2 stats
        gzA = rot.tile([HH, L], BF16, tag="pb")
        gzB = rot.tile([HH, L], BF16, tag="pb")
        x1o = pP.tile([CH, L], BF16, tag="x1o")
        yb = pP.tile([CH, L], BF16, tag="yb")
        ybsq = rot.tile([CH, L], BF16, tag="pb")
        for c0, c1 in MM_CHUNKS:
            cw = c1 - c0
            nc.vector.tensor_mul(gzA[:, c0:c1], ynA[:, c0:c1], zsA[:, c0:c1])
            nc.vector.tensor_mul(gzB[:, c0:c1], ynB[:, c0:c1], zsB[:, c0:c1])
            ps = post_ps.tile([CH, 512], FP32, tag="ps")
            nc.tensor.matmul(ps[:, :cw], cv('outwTa'), gzA[:, c0:c1],
                             start=True, stop=False)
            nc.tensor.matmul(ps[:, :cw], cv('outwTb'), gzB[:, c0:c1],
                             start=False, stop=True)
            nc.scalar.copy(x1o[:, c0:c1], ps[:, :cw])
            nc.vector.tensor_add(yb[:, c0:c1], x1o[:, c0:c1], x2g[:, c0:c1])
            nc.vector.tensor_mul(ybsq[:, c0:c1], yb[:, c0:c1], yb[:, c0:c1])
            ps1 = post_ps.tile([1, 512], FP32, tag="ps")
            nc.tensor.matmul(ps1[:, :cw], ones96[:], yb[:, c0:c1], start=True, stop=True)
            nc.scalar.copy(sA[0:1, c0:c1], ps1[:, :cw])
            ps2 = post_ps.tile([1, 512], FP32, tag="ps")
            nc.tensor.matmul(ps2[:, :cw], ones96[:], ybsq[:, c0:c1], start=True, stop=True)
            nc.scalar.copy(sB[0:1, c0:c1], ps2[:, :cw])
        ln_stats(1.0 / CH)
        ybn = pP.tile([CH, L], BF16, tag="ybn")
        apply_ln([(ybn, yb, 'lng', 'lnb')], 96)

        # CRM
        low_t = rot.tile([48, L], BF16, tag="pb")
        for c0, c1 in MM_CHUNKS:
            nc.sync.dma_start(low_t[:, c0:c1], ybn[48:96, c0:c1])
        upc = pP.tile([24, L], BF16, tag="upc")
        lowc = pP.tile([24, L], BF16, tag="lowc")
        m2cb = pP.tile([24, 5], FP32, tag="m2cb")
        _li = {c0: i for i, (c0, c1) in enumerate(MM_CHUNKS)}.get
        for c0, c1 in MM_CHUNKS:
            ps = post_ps.tile([24, 512], FP32, tag="ps")
            nc.tensor.matmul(ps[:, :c1 - c0], cv('sq1T'), ybn[0:48, c0:c1], start=True, stop=True)
            nc.scalar.copy(upc[:, c0:c1], ps[:, :c1 - c0])
            ps2 = post_ps.tile([24, 512], FP32, tag="ps")
            nc.tensor.matmul(ps2[:, :c1 - c0], cv('sq2T'), low_t[:, c0:c1], start=True, stop=True)
            nc.scalar.activation(lowc[:, c0:c1], ps2[:, :c1 - c0], F.Identity,
                                 accum_out=m2cb[:, _li(c0):_li(c0) + 1])
        upcp = pP.tile([24, LP], BF16, tag="upcp")
        nc.gpsimd.memset(upcp[:], 0.0)
        nc.vector.tensor_copy(hwp(upcp[:])[:, 1:49, 1:49], hw(upc[:]))
        Y1 = pP.tile([CH, L], BF16, tag="Y1")
        m1c = pP.tile([CH, 5], FP32, tag="m1c")
        for ri, (r0, r1) in enumerate(ROW_CHUNKS):
            nr = r1 - r0
            ps = post_ps.tile([CH, 480], FP32, tag="ps")
            for tap in range(9):
                dy, dx = tap // 3, tap % 3
                rhs = hwp(upcp[:])[:, dy + r0:dy + r1, dx:dx + 48]
                nc.tensor.matmul(ps[:, :nr * 48], cvc('gwcT', tap * CH, (tap + 1) * CH),
                                 rhs, start=(tap == 0), stop=False)
            nc.tensor.matmul(ps[:, :nr * 48], cv('pw1T'), upc[:, r0 * 48:r1 * 48],
                             start=False, stop=True)
            nc.scalar.activation(Y1[:, r0 * 48:r1 * 48], ps[:, :nr * 48],
                                 F.Identity, bias=cv('gwcb'),
                                 accum_out=m1c[:, ri:ri + 1])
        Y2a = pP.tile([72, L], BF16, tag="Y2a")
        m2ca = pP.tile([72, 5], FP32, tag="m2ca")
        for ri, (c0, c1) in enumerate(MM_CHUNKS):
            ps = post_ps.tile([72, 512], FP32, tag="ps")
            nc.tensor.matmul(ps[:, :c1 - c0], cv('pw2T'), lowc[:, c0:c1], start=True, stop=True)
            nc.scalar.activation(Y2a[:, c0:c1], ps[:, :c1 - c0], F.Identity,
                                 accum_out=m2ca[:, ri:ri + 1])
        m1 = pP.tile([CH, 1], FP32, tag="m1")
        m2a_s = pP.tile([72, 1], FP32, tag="m2a_s")
        m2b_s = pP.tile([24, 1], FP32, tag="m2b_s")
        nc.vector.reduce_sum(m1[:], m1c[:], axis=mybir.AxisListType.X)
        nc.vector.reduce_sum(m2a_s[:], m2ca[:], axis=mybir.AxisListType.X)
        nc.vector.reduce_sum(m2b_s[:], m2cb[:], axis=mybir.AxisListType.X)
        smf = pP.tile([1, 2 * CH], FP32, tag="smf")
        nc.sync.dma_start(smf[0:1, 0:CH], m1[:, 0:1])
        nc.sync.dma_start(smf[0:1, CH:CH + 72], m2a_s[:, 0:1])
        nc.sync.dma_start(smf[0:1, CH + 72:2 * CH], m2b_s[:, 0:1])
        nc.vector.tensor_scalar(out=smf[:], in0=smf[:], scalar1=1.0 / L,
                                scalar2=None, op0=A.mult)
        mx = pP.tile([1, 1], FP32, tag="mx")
        nc.vector.reduce_max(mx[:], smf[:], axis=mybir.AxisListType.X)
        nc.vector.tensor_scalar(out=mx[:], in0=mx[:], scalar1=-1.0,
                                scalar2=None, op0=A.mult)
        nc.scalar.activation(smf[:], smf[:], F.Exp, bias=mx[0:1, 0:1])
        sm_s = pP.tile([1, 1], FP32, tag="sm_s")
        nc.vector.reduce_sum(sm_s[:], smf[:], axis=mybir.AxisListType.X)
        nc.vector.reciprocal(sm_s[:], sm_s[:])
        nc.vector.tensor_scalar(out=smf[:], in0=smf[:], scalar1=sm_s[0:1, 0:1],
                                scalar2=None, op0=A.mult)
        sm1 = pP.tile([CH, 1], FP32, tag="sm1")
        sm2 = pP.tile([CH, 1], FP32, tag="sm2")
        nc.sync.dma_start(sm1[:, 0:1], smf[0:1, 0:CH])
        nc.sync.dma_start(sm2[:, 0:1], smf[0:1, CH:2 * CH])
        o2f = rot.tile([CH, L], BF16, tag="pb")
        nc.sync.dma_start(o2f[0:72, :], Y2a[:])
        nc.sync.dma_start(o2f[72:96, :], lowc[:])
        o2t = pP.tile([CH, L], BF16, tag="o2t")
        yc = pP.tile([CH, L], BF16, tag="yc")
        outt = pP.tile([COUT, L], FP32, tag="outt")
        for c0, c1 in MM_CHUNKS:
            nc.vector.tensor_scalar(out=o2t[:, c0:c1], in0=o2f[:, c0:c1],
                                    scalar1=sm2[:, 0:1], scalar2=None, op0=A.mult)
            nc.vector.scalar_tensor_tensor(out=yc[:, c0:c1], in0=Y1[:, c0:c1],
                                           scalar=sm1[:, 0:1], in1=o2t[:, c0:c1],
                                           op0=A.mult, op1=A.add)
            ps = post_ps.tile([COUT, 512], FP32, tag="ps")
            nc.tensor.matmul(ps[:, :c1 - c0], cv('finT'), yc[:, c0:c1], start=True, stop=True)
            nc.scalar.activation(outt[:, c0:c1], ps[:, :c1 - c0], F.Identity, bias=cv('finb'))
            nc.sync.dma_start(out_d[:, c0:c1], outt[:, c0:c1])
        po.close()
        glob.close()
    split_multi_waits(nc, max_waits=1)
    return nc


# =============================== host side ==================================

def prep_core_inputs(inputs, b, half):
    import ml_dtypes
    f32 = np.float32
    bf16 = ml_dtypes.bfloat16
    d0 = half * HH

    def asf(a):
        return np.asarray(a, f32)

    bnscale = asf(inputs['bn_g']) / np.sqrt(np.float32(1.0 + 1e-5))
    w1 = asf(inputs['conv1_w'])[:, :, 0, 0] * bnscale[:, None]
    b1 = asf(inputs['conv1_b']) * bnscale + asf(inputs['bn_b'])

    def diag9(w, nch):
        out = np.zeros((nch, 9 * nch), f32)
        w = asf(w)
        for tap in range(9):
            dy, dx = tap // 3, tap % 3
            blk = out[:, tap * nch:(tap + 1) * nch]
            np.fill_diagonal(blk, w[:, 0, dy, dx])
        return out

    sscd = diag9(inputs['ss_conv_w'], DI)        # (192, 9*192)
    sc0 = np.zeros((128, 9 * 128), f32)
    sc1 = np.zeros((64, 9 * 64), f32)
    for tap in range(9):
        blk = sscd[:, tap * DI:(tap + 1) * DI]
        sc0[:, tap * 128:(tap + 1) * 128] = blk[0:128, 0:128]
        sc1[:, tap * 64:(tap + 1) * 64] = blk[128:192, 128:192]

    sel = np.zeros((DI, HH), f32)
    sel[np.arange(d0, d0 + HH), np.arange(HH)] = 1.0

    xp = asf(inputs['ss_xproj_w'])               # (4, 38, 192)
    xpTa = np.zeros((128, K4 * 38), f32)
    xpTb = np.zeros((64, K4 * 38), f32)
    for k in range(K4):
        xpT = xp[k].T                            # (192, 38)
        xpTa[:, k * 38:(k + 1) * 38] = xpT[0:128]
        xpTb[:, k * 38:(k + 1) * 38] = xpT[128:192]

    dtw = asf(inputs['ss_dt_w'])
    dtwT = np.zeros((R, K4 * HH), f32)
    for k in range(K4):
        dtwT[:, k * HH:(k + 1) * HH] = dtw[k][d0:d0 + HH, :].T

    dtb_full = asf(inputs['ss_dt_b'])
    Alog = asf(inputs['ss_Alog']).reshape(K4, DI, N)
    Dv = asf(inputs['ss_D']).reshape(K4, DI)
    dtb_p = np.zeros((128, NT), f32)
    Ap = np.zeros((128, NT * N), f32)
    for (j, o0, o1, k, dd0, dd1) in SECTIONS:
        dtb_p[o0:o1, j] = dtb_full[k, d0 + dd0:d0 + dd1]
        Ap[o0:o1, j * N:(j + 1) * N] = -np.exp(Alog[k, d0 + dd0:d0 + dd1])
    Dsum = Dv[:, d0:d0 + HH].sum(0)[:, None]

    gw = asf(inputs['gwc_w'])
    gT = np.zeros((24, 9 * CH), f32)
    for tap in range(9):
        dy, dx = tap // 3, tap % 3
        blk = np.zeros((24, CH), f32)
        blk[0:12, 0:48] = gw[0:48, :, dy, dx].T
        blk[12:24, 48:96] = gw[48:96, :, dy, dx].T
        gT[:, tap * CH:(tap + 1) * CH] = blk

    owT = asf(inputs['ss_out_w']).T              # (192, 96)
    outn_g = asf(inputs['ss_outn_g'])
    outn_b = asf(inputs['ss_outn_b'])

    vals32 = {
        'b1': b1[:, None],
        'linb': asf(inputs['lin_b'])[:, None],
        'dw1b': asf(inputs['dw1_b'])[:, None],
        'dw2b': asf(inputs['dw2_b'])[:, None],
        'scb0': asf(inputs['ss_conv_b'])[0:128, None],
        'scb1': asf(inputs['ss_conv_b'])[128:192, None],
        'dtb': dtb_p, 'Ap': Ap, 'Dsum': Dsum,
        'outngA': outn_g[0:96, None], 'outngB': outn_g[96:192, None],
        'outnbA': outn_b[0:96, None], 'outnbB': outn_b[96:192, None],
        'ag1b': asf(inputs['ag1_b'])[:, None],
        'ag2b': asf(inputs['ag2_b'])[:, None],
        'lng': asf(inputs['ln_g'])[:, None],
        'lnb': asf(inputs['ln_b'])[:, None],
        'gwcb': asf(inputs['gwc_b'])[:, None],
        'finb': asf(inputs['fin_b'])[:, None],
    }
    valsbf = {
        'w1T': w1.T,
        'linT': asf(inputs['lin_w']).T,
        'dw1dg': diag9(inputs['dw1_w'], CH),
        'dw2dg': diag9(inputs['dw2_w'], CH),
        'inwT': asf(inputs['ss_in_w']).T,        # (96, 384) full z
        'sc0dg': sc0, 'sc1dg': sc1,
        'sel0': sel[0:128], 'sel1': sel[128:192],
        'xpTa': xpTa, 'xpTb': xpTb,
        'dtwT': dtwT,
        'ident': np.eye(128, dtype=f32),
        'outwTa': owT[0:96], 'outwTb': owT[96:192],
        'ag1T': asf(inputs['ag1_w'])[:, :, 0, 0].T,
        'ag2T': asf(inputs['ag2_w'])[:, :, 0, 0].T,
        'sq1T': asf(inputs['sq1_w'])[:, :, 0, 0].T,
        'sq2T': asf(inputs['sq2_w'])[:, :, 0, 0].T,
        'gwcT': gT,
        'pw1T': asf(inputs['pwc1_w'])[:, :, 0, 0].T,
        'pw2T': asf(inputs['pwc2_w'])[:, :, 0, 0].T,
        'finT': asf(inputs['fin_w']).T,
        'bcv': np.concatenate([np.full((1, 96), 1.0 / DI, f32),
                               np.full((1, 96), 1.0 / CH, f32),
                               np.ones((1, 96), f32)], axis=1),
    }

    blob32 = np.zeros((128, W32), f32)
    for nm, p, c in CONSTS_F32:
        o = OFF32[nm][0]
        v = vals32[nm]
        assert v.shape == (p, c), (nm, v.shape, (p, c))
        blob32[0:p, o:o + c] = v
    blobbf = np.zeros((128, WBF), bf16)
    for nm, p, c in CONSTS_BF16:
        o = OFFBF[nm][0]
        v = valsbf[nm]
        assert v.shape == (p, c), (nm, v.shape, (p, c))
        blobbf[0:p, o:o + c] = v.astype(bf16)

    return {
        'x': np.ascontiguousarray(asf(inputs['x'])[b].reshape(CIN, L).astype(bf16)),
        'c32': blob32,
        'cbf': np.ascontiguousarray(blobbf),
    }


_NC_CACHE = {}


def get_nc():
    if 'nc' not in _NC_CACHE:
        _NC_CACHE['nc'] = build_nc()
    return _NC_CACHE['nc']


def kernel(**inputs):
    from concourse.bass_utils import run_bass_kernel_spmd
    nc = get_nc()
    in_maps = [prep_core_inputs(inputs, c // 2, c % 2) for c in range(8)]
    res = run_bass_kernel_spmd(nc, in_maps, core_ids=list(range(8)))
    out = np.zeros((B_, COUT, H, W), np.float32)
    for b in range(B_):
        out[b] = res.results[2 * b]['out'].reshape(COUT, H, W)
    return out


# revision 64
# speedup vs baseline: 1.0735x; 1.0014x over previous
"""Self-contained Trainium2 Bass kernel for the CR-VSS block (8 cores)."""

# ---- TileContext drain-wait patch (walrus 1-wait limit) ----
"""Patch TileContext._drain_and_barrier: the axon-client walrus rejects
instructions carrying >2 sem waits ("Too many sync wait commands" in
setupSyncWait for CTRL structs). Redistribute the exit-drain's waits across
preceding SP nop instructions, each carrying at most MAX_WAITS."""
from concourse.tile import TileContext, ScopedClock

MAX_WAITS = 1


def _patched_drain_and_barrier(self, tick_clock, wait_clock):
    nc = self.nc
    drain_inst = nc.sync.drain()
    wait_clock.add_sem_waits(
        drain_inst.ins, ScopedClock({None: tick_clock.global_clock})
    )

    waits = list(drain_inst.ins.sync_info.on_wait or [])
    if len(waits) > MAX_WAITS:
        bb = nc.cur_bb.bb
        assert bb.instructions[-1] is drain_inst.ins
        # strip waits from the drain, re-emit them on nop carriers
        drain_inst.ins.sync_info.on_wait = waits[:0]
        carriers = []
        import concourse.mybir as mybir
        for i in range(0, len(waits), MAX_WAITS):
            nop = nc.sync.nop(nofuse=True)
            nop.ins.sync_info = mybir.SyncInfo(
                on_wait=waits[i:i + MAX_WAITS], on_update=[]
            )
            carriers.append(nop.ins)
        # move carriers before the drain
        insts = list(bb.instructions)
        assert insts[-len(carriers) - 1] is drain_inst.ins
        reordered = insts[:-len(carriers) - 1] + insts[-len(carriers):] + [drain_inst.ins]
        while len(bb.instructions):
            bb.instructions.pop()
        for x in reordered:
            bb.instructions.append(x)

    nc.all_engine_barrier()
    assert self.sems is not None
    popped = nc._tile_sem_poison_stack.pop()
    assert popped is self._sem_poison
    nc.clear_and_free_semaphores(list(self.sems.allocated().values()))
    nc.all_engine_barrier()


def apply():
    TileContext._drain_and_barrier = _patched_drain_and_barrier


def split_multi_waits(nc, max_waits=1):
    """Post-pass: walrus CTRL codegen rejects instructions with more than
    one sem wait. Move extra waits onto same-engine NoOp carriers."""
    import concourse.mybir as mybir
    for f in nc.m.functions:
        for bb in f.blocks:
            insts = list(bb.instructions)
            out = []
            changed = False
            for ins in insts:
                si = ins.sync_info
                if si is not None and si.on_wait and len(si.on_wait) > max_waits:
                    waits = list(si.on_wait)
                    for i, w in enumerate(waits[max_waits:]):
                        nop = mybir.InstNoOp.__new__(
                            mybir.InstNoOp, name=f"{ins.name}-xw{i}", ins=[], outs=[])
                        nop.engine = ins.engine
                        nop.sync_info = mybir.SyncInfo(on_wait=[w], on_update=[])
                        out.append(nop)
                    ins.sync_info = mybir.SyncInfo(
                        on_wait=waits[:max_waits],
                        on_update=list(si.on_update or []))
                    changed = True
                out.append(ins)
            if changed:
                while len(bb.instructions):
                    bb.instructions.pop()
                for x in out:
                    bb.instructions.append(x)

apply()

# ---- kernel ----
"""Trainium2 Bass kernel for nn_CR_VSS (VSS block with SS2D selective scan).

Sharding: 8 cores = 4 samples x 2 d_inner-halves. Each core runs the full
pre-stage for its sample, scans its 96-channel d-half across all 4
cross-scan directions (packed into 3x128-partition tiles), then the pair
exchanges y-halves with ONE AllGather; LN + out-proj + post-stage run
locally (z is computed full-width in the in-proj so no second collective).

Scan: h_t = exp(A*delta_t)*h_{t-1} + delta_t*u_t*B_t per (k,d,n) via
tensor_tensor_scan; n in groups of 4 with batched B/C partition-broadcast
DMAs (double-buffered); y accumulated over n with identity-lhsT PSUM
matmuls, merged into pixel-order ysum straight from PSUM per t-chunk.
"""
import numpy as np
from contextlib import ExitStack

import concourse.bass as bass
import concourse.mybir as mybir

F = mybir.ActivationFunctionType
A = mybir.AluOpType
FP32 = mybir.dt.float32
BF16 = mybir.dt.bfloat16

B_, CIN, CH, COUT, H, W = 4, 96, 96, 96, 48, 48
DI, N, R, K4 = 192, 16, 6, 4
L = H * W               # 2304
HH = 96                 # d-half per core
NT = 3                  # packed (k,d) tiles: 4*96 = 384 = 3*128
HP = 50
LP = 2500
TC = 768                # scan t-chunk (16 rows of 48)
TCH = [(0, 768), (768, 1536), (1536, 2304)]
NG = 4                  # scan n-group (broadcast batch)

# packed (k,d) rows -> (tile j, offset): sections (j, o0, o1, k, d0, d1).
# Section offsets are all 0/32/64 so PE matmuls can write them directly.
SECTIONS = [
    (0, 0, 32, 1, 0, 32),
    (0, 32, 128, 0, 0, 96),
    (1, 0, 64, 1, 32, 96),
    (1, 64, 128, 2, 0, 64),
    (2, 0, 32, 2, 64, 96),
    (2, 32, 128, 3, 0, 96),
]

MM_CHUNKS = [(0, 512), (512, 1024), (1024, 1536), (1536, 2048), (2048, 2304)]
ROW_CHUNKS = [(0, 10), (10, 20), (20, 30), (30, 40), (40, 48)]
SUBS768 = [(0, 512), (512, 768)]
INW_BLOCKS = [(0, 128), (128, 256), (256, 384)]

REPLICA_GROUPS = [[0, 1], [2, 3], [4, 5], [6, 7]]

# ---- const blobs (shared layout between host packing and kernel views) ----
CONSTS_F32 = [
    ('b1', 96, 1), ('linb', 96, 1),
    ('dw1b', 96, 1), ('dw2b', 96, 1),
    ('scb0', 128, 1), ('scb1', 64, 1),
    ('dtb', 128, 3), ('Ap', 128, 48), ('Dsum', 96, 1),
    ('outngA', 96, 1), ('outngB', 96, 1), ('outnbA', 96, 1), ('outnbB', 96, 1),
    ('ag1b', 48, 1), ('ag2b', 96, 1), ('lng', 96, 1), ('lnb', 96, 1),
    ('gwcb', 96, 1), ('finb', 96, 1),
]
CONSTS_BF16 = [
    ('w1T', 96, 96), ('linT', 96, 96),
    ('dw1dg', 96, 864), ('dw2dg', 96, 864),
    ('inwT', 96, 384),
    ('sc0dg', 128, 1152), ('sc1dg', 64, 576),
    ('sel0', 128, 96), ('sel1', 64, 96),
    ('xpTa', 128, 152), ('xpTb', 64, 152),
    ('dtwT', 6, 384),
    ('ident', 128, 128),
    ('outwTa', 96, 96), ('outwTb', 96, 96),
    ('ag1T', 96, 48), ('ag2T', 48, 96),
    ('sq1T', 48, 24), ('sq2T', 48, 24),
    ('gwcT', 24, 864), ('pw1T', 24, 96), ('pw2T', 24, 72),
    ('finT', 96, 96), ('bcv', 1, 288),
]

OFF32 = {}
_o = 0
for _nm, _p, _c in CONSTS_F32:
    OFF32[_nm] = (_o, _p, _c)
    _o += _c
W32 = _o
OFFBF = {}
_o = 0
for _nm, _p, _c in CONSTS_BF16:
    OFFBF[_nm] = (_o, _p, _c)
    _o += _c
WBF = _o


def build_nc():
    nc = bass.Bass(trn_type="TRN2", num_devices=8)

    x_d = nc.dram_tensor("x", [CIN, L], BF16, kind="ExternalInput")
    c32_d = nc.dram_tensor("c32", [128, W32], FP32, kind="ExternalInput")
    cbf_d = nc.dram_tensor("cbf", [128, WBF], BF16, kind="ExternalInput")
    out_d = nc.dram_tensor("out", [COUT, L], FP32, kind="ExternalOutput")

    B_dram = nc.dram_tensor("B_dram", [K4 * N, L], BF16)
    C_dram = nc.dram_tensor("C_dram", [K4 * N, L], BF16)
    y_dram = nc.dram_tensor("y_dram", [HH, L], BF16)
    yg_dram = nc.dram_tensor("yg_dram", [DI, L], BF16)
    st_dram = nc.dram_tensor("st_dram", [2, L], BF16)

    def hw(ap):
        return ap.rearrange("p (h w) -> p h w", h=H)

    def hwp(ap):
        return ap.rearrange("p (h w) -> p h w", h=HP)

    def whv(ap):
        return ap.rearrange("p (h w) -> p w h", h=H)

    with TileContext(nc) as tc:
        glob = ExitStack()
        cst = glob.enter_context(tc.tile_pool(name="cst", bufs=1))
        lngA = glob.enter_context(tc.tile_pool(name="lngA", bufs=1))

        cst32 = cst.tile([128, W32], FP32, tag="cst32")
        cstbf = cst.tile([128, WBF], BF16, tag="cstbf")
        nc.sync.dma_start(cst32[:], c32_d[:])
        nc.sync.dma_start(cstbf[:], cbf_d[:])

        def cvc(nm, a0=0, a1=None, p0=0, p1=None):
            d, tile = (OFF32, cst32) if nm in OFF32 else (OFFBF, cstbf)
            o, p, c = d[nm]
            if a1 is None:
                a1 = c
            if p1 is None:
                p1 = p
            return tile[p0:p1, o + a0:o + a1]

        cv = cvc

        ones96 = cst.tile([HH, 1], BF16, tag="ones96")
        nc.vector.memset(ones96[:], 1.0)

        # long-lived across phases
        z0 = lngA.tile([128, L], BF16, tag="z0")     # z rows 0:128
        z1 = lngA.tile([64, L], BF16, tag="z1")      # z rows 128:192
        x2 = lngA.tile([CH, L], BF16, tag="x2")
        lngB = ExitStack()
        lngB_p = lngB.enter_context(tc.tile_pool(name="lngB_p", bufs=1))
        xch = lngB_p.tile([HH, L], BF16, tag="xch")
        dp = [lngB_p.tile([128, L], BF16, tag=f"dp{j}", name=f"dp{j}") for j in range(NT)]
        # xsp holds packed scan-order xs, overwritten in place with delta*u
        xsp = [lngB_p.tile([128, L], BF16, tag=f"xsp{j}", name=f"xsp{j}") for j in range(NT)]
        ysum = lngB_p.tile([HH, L], BF16, tag="ysum")

        # ================= pre-stage =================
        pre = ExitStack()
        pre_ps = pre.enter_context(tc.tile_pool(name="pre_ps", bufs=6, space="PSUM"))
        pA = pre.enter_context(tc.tile_pool(name="pA", bufs=1))
        pB = pre.enter_context(tc.tile_pool(name="pB", bufs=1))

        xt = pA.tile([CIN, L], BF16, tag="xt")
        nc.sync.dma_start(xt[:], x_d[:])

        # conv1x1 (+folded BN) + ReLU
        h1 = pA.tile([CH, L], BF16, tag="h1")
        for c0, c1 in MM_CHUNKS:
            ps = pre_ps.tile([CH, 512], FP32, tag="ps")
            nc.tensor.matmul(ps[:, :c1 - c0], cv('w1T'), xt[:, c0:c1], start=True, stop=True)
            nc.scalar.activation(h1[:, c0:c1], ps[:, :c1 - c0], F.Relu, bias=cv('b1'))
        # token linear
        h2 = pA.tile([CH, L], BF16, tag="h2")
        for c0, c1 in MM_CHUNKS:
            ps = pre_ps.tile([CH, 512], FP32, tag="ps")
            nc.tensor.matmul(ps[:, :c1 - c0], cv('linT'), h1[:, c0:c1], start=True, stop=True)
            nc.vector.tensor_scalar(out=h2[:, c0:c1], in0=ps[:, :c1 - c0],
                                    scalar1=cv('linb'), scalar2=None, op0=A.add)
        h2p = pA.tile([CH, LP], BF16, tag="h2p")
        nc.gpsimd.memset(h2p[:], 0.0)
        for (r0, r1) in ROW_CHUNKS:
            nc.vector.tensor_copy(hwp(h2p[:])[:, r0 + 1:r1 + 1, 1:49],
                                  hw(h2[:])[:, r0:r1, :])

        def dwconv(dst, src_p, dgname, biasname, nch):
            for (r0, r1) in ROW_CHUNKS:
                nr = r1 - r0
                ps = pre_ps.tile([128, 480], FP32, tag="ps")
                for tap in range(9):
                    dy, dx = tap // 3, tap % 3
                    rhs = hwp(src_p[:])[:, dy + r0:dy + r1, dx:dx + 48]
                    nc.tensor.matmul(ps[:nch, :nr * 48],
                                     cvc(dgname, tap * nch, (tap + 1) * nch),
                                     rhs, start=(tap == 0), stop=(tap == 8))
                nc.scalar.activation(dst[:, r0 * 48:r1 * 48], ps[:nch, :nr * 48],
                                     F.Silu, bias=cv(biasname))

        x1 = pB.tile([CH, L], BF16, tag="x1")
        dwconv(x1, h2p, 'dw1dg', 'dw1b', CH)

        # in-proj: xi (192) + FULL z (192)
        xi0 = pB.tile([128, L], BF16, tag="xi0")
        xi1 = pB.tile([64, L], BF16, tag="xi1")
        for mb, (m0, m1) in enumerate(INW_BLOCKS):
            for c0, c1 in MM_CHUNKS:
                ps = pre_ps.tile([128, 512], FP32, tag="ps")
                nc.tensor.matmul(ps[:m1 - m0, :c1 - c0], cvc('inwT', m0, m1),
                                 x1[:, c0:c1], start=True, stop=True)
                if mb == 0:
                    nc.vector.tensor_copy(xi0[:, c0:c1], ps[:128, :c1 - c0])
                elif mb == 1:
                    nc.scalar.copy(xi1[:, c0:c1], ps[0:64, :c1 - c0])
                    nc.scalar.copy(z0[0:64, c0:c1], ps[64:128, :c1 - c0])
                else:
                    nc.scalar.copy(z0[64:128, c0:c1], ps[0:64, :c1 - c0])
                    nc.scalar.copy(z1[0:64, c0:c1], ps[64:128, :c1 - c0])

        xi0p = pB.tile([128, LP], BF16, tag="xi0p")
        xi1p = pB.tile([64, LP], BF16, tag="xi1p")
        nc.gpsimd.memset(xi0p[:], 0.0)
        nc.gpsimd.memset(xi1p[:], 0.0)
        for (r0, r1) in ROW_CHUNKS:
            nc.vector.tensor_copy(hwp(xi0p[:])[:, r0 + 1:r1 + 1, 1:49],
                                  hw(xi0[:])[:, r0:r1, :])
            nc.vector.tensor_copy(hwp(xi1p[:])[:, r0 + 1:r1 + 1, 1:49],
                                  hw(xi1[:])[:, r0:r1, :])
        xc0 = pB.tile([128, L], BF16, tag="xc0")
        xc1 = pB.tile([64, L], BF16, tag="xc1")
        dwconv(xc0, xi0p, 'sc0dg', 'scb0', 128)
        dwconv(xc1, xi1p, 'sc1dg', 'scb1', 64)

        # d-half extraction + wh copy
        for c0, c1 in MM_CHUNKS:
            ps = pre_ps.tile([HH, 512], FP32, tag="ps")
            nc.tensor.matmul(ps[:, :c1 - c0], cv('sel0'), xc0[:, c0:c1], start=True, stop=False)
            nc.tensor.matmul(ps[:, :c1 - c0], cv('sel1'), xc1[:, c0:c1], start=False, stop=True)
            nc.vector.tensor_copy(xch[:, c0:c1], ps[:, :c1 - c0])
        xwhh = pB.tile([HH, L], BF16, tag="xwhh")
        for (t0, t1) in TCH:
            w0, w1 = t0 // 48, t1 // 48
            nc.vector.tensor_copy(hw(xwhh[:])[:, w0:w1, :],
                                  whv(xch[:])[:, w0:w1, :])

        # xproj (compact 38 rows: 0:6 dts, 6:22 B, 22:38 C) in scan order
        def xc_read(k, c0, c1):
            if k == 0:
                return (xc0[:, c0:c1], xc1[:, c0:c1])
            if k == 1:
                return (whv(xc0[:])[:, c0 // 48:c1 // 48, :],
                        whv(xc1[:])[:, c0 // 48:c1 // 48, :])
            if k == 2:
                return (xc0[:, L - c1:L - c0][:, ::-1],
                        xc1[:, L - c1:L - c0][:, ::-1])
            r0 = whv(xc0[:])[:, (L - c1) // 48:(L - c0) // 48, :][:, ::-1, ::-1]
            r1 = whv(xc1[:])[:, (L - c1) // 48:(L - c0) // 48, :][:, ::-1, ::-1]
            return (r0, r1)

        # row-chunk outer so all 4 directions' early columns finish first;
        # B/C are written to DRAM per scan chunk so ci=0 broadcasts can
        # start while xproj still works on later chunks.
        stage = [pB.tile([38, L], BF16, tag=f"stg{k}", name=f"stg{k}") for k in range(K4)]
        done_w = 0
        for ri, (rr0, rr1) in enumerate(ROW_CHUNKS):
            c0, c1 = rr0 * 48, rr1 * 48
            nf = c1 - c0
            for k in range(K4):
                ra, rb = xc_read(k, c0, c1)
                ps = pre_ps.tile([38, 480], FP32, tag="ps")
                nc.tensor.matmul(ps[:, :nf], cvc('xpTa', k * 38, (k + 1) * 38), ra,
                                 start=True, stop=False)
                nc.tensor.matmul(ps[:, :nf], cvc('xpTb', k * 38, (k + 1) * 38), rb,
                                 start=False, stop=True)
                nc.vector.tensor_copy(stage[k][:, c0:c1], ps[:, :nf])
            while done_w < len(TCH) and TCH[done_w][1] <= c1:
                t0, t1 = TCH[done_w]
                for k in range(K4):
                    nc.sync.dma_start(B_dram[k * N:(k + 1) * N, t0:t1],
                                      stage[k][6:22, t0:t1])
                    nc.sync.dma_start(C_dram[k * N:(k + 1) * N, t0:t1],
                                      stage[k][22:38, t0:t1])
                done_w += 1

        # delta: packed matmuls then softplus on full 128-partition tiles
        def mm_windows(a0, a1):
            if a0 == 0:
                return [(0, a1)]
            res = []
            x = a0
            while x < a1:
                if x % 64 == 32:
                    e = min(a1, x + 32)
                else:  # x == 64
                    e = min(a1, 128)
                res.append((x, e))
                x = e
            return res

        for (cc0, cc1) in MM_CHUNKS:
            cw = cc1 - cc0
            for j in range(NT):
                ex = pre_ps.tile([128, 512], FP32, tag="ps")
                for (jj, o0, o1, k, d0, d1) in SECTIONS:
                    if jj != j:
                        continue
                    for (w0, w1) in mm_windows(o0, o1):
                        dd0 = d0 + (w0 - o0)
                        dd1 = d0 + (w1 - o0)
                        nc.tensor.matmul(ex[w0:w1, :cw],
                                         cvc('dtwT', k * 96 + dd0, k * 96 + dd1),
                                         stage[k][0:6, cc0:cc1], start=True, stop=True)
                # softplus(x+b) = ln(1 + exp(x+b)) (no softplus act table on HW)
                ex2 = pre_ps.tile([128, 512], FP32, tag="ps")
                nc.scalar.activation(ex2[:, :cw], ex[:, :cw], F.Exp,
                                     bias=cvc('dtb', j, j + 1))
                nc.scalar.activation(dp[j][:, cc0:cc1], ex2[:, :cw], F.Ln, bias=1.0)

        # pack scan-order xs (Act copies handle partition shift + flips),
        # then overwrite in place with delta*u = dp*xs.
        # Act partition windows must not cross engine block boundaries on
        # EITHER side: allowed starts 0/32/64/96; a start-32 window may not
        # cross 64. split2 chops a shifted copy accordingly.
        def _legal_span(s):
            return 32 if s == 32 else 128 - s if s else 128

        def split2(o0, i0, ln):
            res = []
            x = 0
            while x < ln:
                step = min(ln - x, _legal_span(o0 + x), _legal_span(i0 + x))
                res.append((x, x + step))
                x += step
            return res

        for (t0, t1) in TCH:
            for (j, o0, o1, k, d0, d1) in SECTIONS:
                v = xwhh if k in (1, 3) else xch
                if k < 2:
                    # forward sections: contiguous rows, cheap DMA shift
                    nc.sync.dma_start(xsp[j][o0:o1, t0:t1], v[d0:d1, t0:t1])
                    continue
                for (w0, w1) in split2(o0, d0, o1 - o0):
                    nc.scalar.copy(xsp[j][o0 + w0:o0 + w1, t0:t1],
                                   v[d0 + w0:d0 + w1, ::-1][:, t0:t1])
            for j in range(NT):
                nc.vector.tensor_mul(xsp[j][:, t0:t1], dp[j][:, t0:t1],
                                     xsp[j][:, t0:t1])

        # branch-2 dwconv last: x2 is only needed in the post stage, so
        # this fills PE/Act slack once the scan inputs are queued
        dwconv(x2, h2p, 'dw2dg', 'dw2b', CH)

        pre.close()

        # ================= scan =================
        sc = ExitStack()
        bbp = sc.enter_context(tc.tile_pool(name="bbp", bufs=2))
        spool = sc.enter_context(tc.tile_pool(name="spool", bufs=2))
        scan_ps = sc.enter_context(tc.tile_pool(name="scan_ps", bufs=1, space="PSUM"))
        stp = sc.enter_context(tc.tile_pool(name="stp", bufs=1))
        state = [stp.tile([128, N], FP32, tag=f"st{j}", name=f"state{j}")
                 for j in range(NT)]
        yd = [stp.tile([HH, L], BF16, tag=f"yd{k}", name=f"yd{k}") for k in range(K4)]

        pending_drain = None
        for ci, (c0, c1) in enumerate(TCH):
            ypsum = [scan_ps.tile([128, TC], FP32, tag=f"yps{j}", name=f"yps{j}_{ci}")
                     for j in range(NT)]
            for g in range(K4):
                Bb = [bbp.tile([128, NG * TC], BF16, tag=f"Bb{j}", name=f"Bb{j}_{ci}_{g}")
                      for j in range(NT)]
                Cb = [bbp.tile([128, NG * TC], BF16, tag=f"Cb{j}", name=f"Cb{j}_{ci}_{g}")
                      for j in range(NT)]
                for (j, o0, o1, k, d0, d1) in SECTIONS:
                    nc.sync.dma_start(
                        Bb[j][o0:o1, :],
                        B_dram[k * N + NG * g:k * N + NG * g + NG, c0:c1]
                        .partition_broadcast(o1 - o0))
                    nc.scalar.dma_start(
                        Cb[j][o0:o1, :],
                        C_dram[k * N + NG * g:k * N + NG * g + NG, c0:c1]
                        .partition_broadcast(o1 - o0))
                if pending_drain is not None:
                    pending_drain()
                    pending_drain = None
                for n4 in range(NG):
                    n = NG * g + n4
                    for j in range(NT):
                        at = spool.tile([128, TC], BF16, tag=f"at{j}", name=f"at{j}")
                        nc.scalar.activation(at[:], dp[j][:, c0:c1], F.Exp,
                                             scale=cvc('Ap', j * N + n, j * N + n + 1))
                        bt = spool.tile([128, TC], BF16, tag=f"bt{j}", name=f"bt{j}")
                        nc.vector.tensor_mul(bt[:], xsp[j][:, c0:c1],
                                             Bb[j][:, n4 * TC:(n4 + 1) * TC])
                        ht = spool.tile([128, TC], BF16, tag=f"ht{j}", name=f"ht{j}", bufs=3)
                        if ci > 0:
                            # fold carry state into bt[0] so the scan can use
                            # the cheap zero-init form
                            nc.vector.scalar_tensor_tensor(
                                out=bt[:, 0:1], in0=at[:, 0:1],
                                scalar=state[j][:, n:n + 1], in1=bt[:, 0:1],
                                op0=A.mult, op1=A.add)
                        nc.vector.tensor_tensor_scan(ht[:], at[:], bt[:], 0.0,
                                                     A.mult, A.add)
                        if ci < 2:
                            nc.vector.tensor_copy(state[j][:, n:n + 1], ht[:, TC - 1:TC])
                        gt = spool.tile([128, TC], BF16, tag=f"gt{j}", name=f"gt{j}")
                        nc.vector.tensor_mul(gt[:], ht[:],
                                             Cb[j][:, n4 * TC:(n4 + 1) * TC])
                        for (s0, s1) in SUBS768:
                            nc.tensor.matmul(ypsum[j][:, s0:s1], cv('ident'),
                                             gt[:, s0:s1],
                                             start=(n == 0), stop=(n == N - 1))
            # drain this chunk's PSUM into per-direction scan-order tiles
            # (Act copies allow the partition shift). Deferred past the next
            # chunk's broadcast issue so the boundary doesn't stall Act.
            def _drain(yps=ypsum, cc0=c0, cc1=c1):
                for (j, o0, o1, k, d0, d1) in SECTIONS:
                    for (w0, w1) in split2(d0, o0, d1 - d0):
                        nc.scalar.copy(yd[k][d0 + w0:d0 + w1, cc0:cc1],
                                       yps[j][o0 + w0:o0 + w1, :])
            pending_drain = _drain
        if pending_drain is not None:
            pending_drain()
            pending_drain = None
        # merge directions into pixel order + D*u term
        tmp96 = stp.tile([HH, L], BF16, tag="tmp96")
        nc.vector.tensor_add(ysum[:], yd[0][:], yd[2][:, ::-1])
        nc.vector.tensor_add(tmp96[:], yd[1][:], yd[3][:, ::-1])
        nc.vector.tensor_add(ysum[:], ysum[:], whv(tmp96[:]))
        nc.vector.scalar_tensor_tensor(out=ysum[:], in0=xch[:], scalar=cv('Dsum'),
                                       in1=ysum[:], op0=A.mult, op1=A.add)
        nc.sync.dma_start(y_dram[:], ysum[:])
        nc.gpsimd.collective_compute(
            "AllGather", A.bypass, replica_groups=REPLICA_GROUPS,
            ins=[y_dram[:]], outs=[yg_dram[:]])
        sc.close()
        lngB.close()

        # ================= post =================
        po = ExitStack()
        post_ps = po.enter_context(tc.tile_pool(name="post_ps", bufs=6, space="PSUM"))
        pP = po.enter_context(tc.tile_pool(name="pP", bufs=1))
        rot = po.enter_context(tc.tile_pool(name="rot", bufs=5))

        # branch 2 + silu(z): no dep on the collective, runs under it
        g1 = rot.tile([48, L], BF16, tag="pb")
        for c0, c1 in MM_CHUNKS:
            ps = post_ps.tile([48, 512], FP32, tag="ps")
            nc.tensor.matmul(ps[:, :c1 - c0], cv('ag1T'), x2[:, c0:c1], start=True, stop=True)
            nc.scalar.activation(g1[:, c0:c1], ps[:, :c1 - c0], F.Relu, bias=cv('ag1b'))
        gat = rot.tile([CH, L], BF16, tag="pb")
        for c0, c1 in MM_CHUNKS:
            ps = post_ps.tile([CH, 512], FP32, tag="ps")
            nc.tensor.matmul(ps[:, :c1 - c0], cv('ag2T'), g1[:, c0:c1], start=True, stop=True)
            nc.scalar.activation(gat[:, c0:c1], ps[:, :c1 - c0], F.Sigmoid, bias=cv('ag2b'))
        x2g = pP.tile([CH, L], BF16, tag="x2g")
        nc.vector.tensor_mul(x2g[:], x2[:], gat[:])

        zsA = pP.tile([HH, L], BF16, tag="zsA")
        zsB = pP.tile([HH, L], BF16, tag="zsB")
        nc.scalar.activation(zsA[:], z0[0:96, :], F.Silu)
        nc.scalar.activation(zsB[0:32, :], z0[96:128, :], F.Silu)
        nc.scalar.activation(zsB[32:64, :], z1[0:32, :], F.Silu)
        nc.scalar.activation(zsB[64:96, :], z1[32:64, :], F.Silu)

        ygA = pP.tile([HH, L], BF16, tag="ygA")
        ygB = pP.tile([HH, L], BF16, tag="ygB")
        nc.sync.dma_start(ygA[:], yg_dram[0:HH, :])
        nc.sync.dma_start(ygB[:], yg_dram[HH:DI, :])

        sA = pP.tile([1, L], BF16, tag="sA")
        sB = pP.tile([1, L], BF16, tag="sB")
        sM = pP.tile([1, L], BF16, tag="sM")

        def ln_stats(cinv):
            # in: sA=raw sum, sB=raw sumsq; leaves rstd in sB (sA stays raw sum)
            with nc.allow_low_precision(reason="LN stats kept bf16; rel-err verified"):
                nc.scalar.activation(sM[:], sA[:], F.Square, scale=cinv)
                nc.vector.scalar_tensor_tensor(out=sB[:], in0=sB[:], scalar=cinv,
                                               in1=sM[:], op0=A.mult, op1=A.subtract)
                nc.vector.tensor_scalar(out=sB[:], in0=sB[:], scalar1=1e-5,
                                        scalar2=None, op0=A.add)
                nc.vector.reciprocal(sB[:], sB[:])
                nc.scalar.activation(sB[:], sB[:], F.Sqrt)

        # LayerNorm over full DI (local stats via ones-matmul), fully
        # chunk-pipelined behind the per-chunk yg reads
        ysqA = rot.tile([HH, L], BF16, tag="pb")
        ysqB = rot.tile([HH, L], BF16, tag="pb")
        for c, (c0, c1) in enumerate(MM_CHUNKS):
            nc.vector.tensor_mul(ysqA[:, c0:c1], ygA[:, c0:c1], ygA[:, c0:c1])
            nc.vector.tensor_mul(ysqB[:, c0:c1], ygB[:, c0:c1], ygB[:, c0:c1])
            ps = post_ps.tile([1, 512], FP32, tag="ps")
            nc.tensor.matmul(ps[:, :c1 - c0], ones96[:], ygA[:, c0:c1], start=True, stop=False)
            nc.tensor.matmul(ps[:, :c1 - c0], ones96[:], ygB[:, c0:c1], start=False, stop=True)
            nc.scalar.copy(sA[0:1, c0:c1], ps[:, :c1 - c0])
            ps2 = post_ps.tile([1, 512], FP32, tag="ps")
            nc.tensor.matmul(ps2[:, :c1 - c0], ones96[:], ysqA[:, c0:c1], start=True, stop=False)
            nc.tensor.matmul(ps2[:, :c1 - c0], ones96[:], ysqB[:, c0:c1], start=False, stop=True)
            nc.scalar.copy(sB[0:1, c0:c1], ps2[:, :c1 - c0])

        ln_stats(1.0 / DI)

        def apply_ln(pairs, bco):
            # pairs: list of (dst, src, gname, bname); bco: bcv column offset
            # holding 1/DI or 1/CH (folds the mean division into the
            # broadcast lhsT)
            for c0, c1 in MM_CHUNKS:
                cw = c1 - c0
                psm = post_ps.tile([HH, 512], FP32, tag="ps")
                nc.tensor.matmul(psm[:, :cw], cvc('bcv', bco, bco + HH),
                                 sA[:, c0:c1], start=True, stop=True)
                psr = post_ps.tile([HH, 512], FP32, tag="ps")
                nc.tensor.matmul(psr[:, :cw], cvc('bcv', 192, 192 + HH),
                                 sB[:, c0:c1], start=True, stop=True)
                for (dst, srct, gname, bname) in pairs:
                    nc.vector.tensor_sub(dst[:, c0:c1], srct[:, c0:c1],
                                         psm[:, :cw])
                    nc.vector.tensor_mul(dst[:, c0:c1], dst[:, c0:c1],
                                         psr[:, :cw])
                    nc.vector.tensor_scalar(out=dst[:, c0:c1], in0=dst[:, c0:c1],
                                            scalar1=cv(gname), scalar2=cv(bname),
                                            op0=A.mult, op1=A.add)

        ynA = rot.tile([HH, L], BF16, tag="pb")
        ynB = rot.tile([HH, L], BF16, tag="pb")
        apply_ln([(ynA, ygA, 'outngA', 'outnbA'),
                  (ynB, ygB, 'outngB', 'outnbB')], 0)

        # fused chunk pipeline: ygz -> out-proj -> yb -> ybsq -> LN2 stats
        gzA = rot.tile([HH, L], BF16, tag="pb")
        gzB = rot.tile([HH, L], BF16, tag="pb")
        x1o = pP.tile([CH, L], BF16, tag="x1o")
        yb = pP.tile([CH, L], BF16, tag="yb")
        ybsq = rot.tile([CH, L], BF16, tag="pb")
        for c0, c1 in MM_CHUNKS:
            cw = c1 - c0
            nc.vector.tensor_mul(gzA[:, c0:c1], ynA[:, c0:c1], zsA[:, c0:c1])
            nc.vector.tensor_mul(gzB[:, c0:c1], ynB[:, c0:c1], zsB[:, c0:c1])
            ps = post_ps.tile([CH, 512], FP32, tag="ps")
            nc.tensor.matmul(ps[:, :cw], cv('outwTa'), gzA[:, c0:c1],
                             start=True, stop=False)
            nc.tensor.matmul(ps[:, :cw], cv('outwTb'), gzB[:, c0:c1],
                             start=False, stop=True)
            nc.scalar.copy(x1o[:, c0:c1], ps[:, :cw])
            nc.vector.tensor_add(yb[:, c0:c1], x1o[:, c0:c1], x2g[:, c0:c1])
            nc.vector.tensor_mul(ybsq[:, c0:c1], yb[:, c0:c1], yb[:, c0:c1])
            ps1 = post_ps.tile([1, 512], FP32, tag="ps")
            nc.tensor.matmul(ps1[:, :cw], ones96[:], yb[:, c0:c1], start=True, stop=True)
            nc.scalar.copy(sA[0:1, c0:c1], ps1[:, :cw])
            ps2 = post_ps.tile([1, 512], FP32, tag="ps")
            nc.tensor.matmul(ps2[:, :cw], ones96[:], ybsq[:, c0:c1], start=True, stop=True)
            nc.scalar.copy(sB[0:1, c0:c1], ps2[:, :cw])
        ln_stats(1.0 / CH)
        ybn = pP.tile([CH, L], BF16, tag="ybn")
        apply_ln([(ybn, yb, 'lng', 'lnb')], 96)

        # CRM
        low_t = rot.tile([48, L], BF16, tag="pb")
        for c0, c1 in MM_CHUNKS:
            nc.sync.dma_start(low_t[:, c0:c1], ybn[48:96, c0:c1])
        upc = pP.tile([24, L], BF16, tag="upc")
        lowc = pP.tile([24, L], BF16, tag="lowc")
        m2cb = pP.tile([24, 5], FP32, tag="m2cb")
        _li = {c0: i for i, (c0, c1) in enumerate(MM_CHUNKS)}.get
        for c0, c1 in MM_CHUNKS:
            ps = post_ps.tile([24, 512], FP32, tag="ps")
            nc.tensor.matmul(ps[:, :c1 - c0], cv('sq1T'), ybn[0:48, c0:c1], start=True, stop=True)
            nc.scalar.copy(upc[:, c0:c1], ps[:, :c1 - c0])
            ps2 = post_ps.tile([24, 512], FP32, tag="ps")
            nc.tensor.matmul(ps2[:, :c1 - c0], cv('sq2T'), low_t[:, c0:c1], start=True, stop=True)
            nc.scalar.activation(lowc[:, c0:c1], ps2[:, :c1 - c0], F.Identity,
                                 accum_out=m2cb[:, _li(c0):_li(c0) + 1])
        upcp = pP.tile([24, LP], BF16, tag="upcp")
        nc.gpsimd.memset(upcp[:], 0.0)
        nc.vector.tensor_copy(hwp(upcp[:])[:, 1:49, 1:49], hw(upc[:]))
        Y1 = pP.tile([CH, L], BF16, tag="Y1")
        m1c = pP.tile([CH, 5], FP32, tag="m1c")
        for ri, (r0, r1) in enumerate(ROW_CHUNKS):
            nr = r1 - r0
            ps = post_ps.tile([CH, 480], FP32, tag="ps")
            for tap in range(9):
                dy, dx = tap // 3, tap % 3
                rhs = hwp(upcp[:])[:, dy + r0:dy + r1, dx:dx + 48]
                nc.tensor.matmul(ps[:, :nr * 48], cvc('gwcT', tap * CH, (tap + 1) * CH),
                                 rhs, start=(tap == 0), stop=False)
            nc.tensor.matmul(ps[:, :nr * 48], cv('pw1T'), upc[:, r0 * 48:r1 * 48],
                             start=False, stop=True)
            nc.scalar.activation(Y1[:, r0 * 48:r1 * 48], ps[:, :nr * 48],
                                 F.Identity, bias=cv('gwcb'),
                                 accum_out=m1c[:, ri:ri + 1])
        Y2a = pP.tile([72, L], BF16, tag="Y2a")
        m2ca = pP.tile([72, 5], FP32, tag="m2ca")
        for ri, (c0, c1) in enumerate(MM_CHUNKS):
            ps = post_ps.tile([72, 512], FP32, tag="ps")
            nc.tensor.matmul(ps[:, :c1 - c0], cv('pw2T'), lowc[:, c0:c1], start=True, stop=True)
            nc.scalar.activation(Y2a[:, c0:c1], ps[:, :c1 - c0], F.Identity,
                                 accum_out=m2ca[:, ri:ri + 1])
        m1 = pP.tile([CH, 1], FP32, tag="m1")
        m2a_s = pP.tile([72, 1], FP32, tag="m2a_s")
        m2b_s = pP.tile([24, 1], FP32, tag="m2b_s")
        nc.vector.reduce_sum(m1[:], m1c[:], axis=mybir.AxisListType.X)
        nc.vector.reduce_sum(m2a_s[:], m2ca[:], axis=mybir.AxisListType.X)
        nc.vector.reduce_sum(m2b_s[:], m2cb[:], axis=mybir.AxisListType.X)
        smf = pP.tile([1, 2 * CH], FP32, tag="smf")
        nc.sync.dma_start(smf[0:1, 0:CH], m1[:, 0:1])
        nc.sync.dma_start(smf[0:1, CH:CH + 72], m2a_s[:, 0:1])
        nc.sync.dma_start(smf[0:1, CH + 72:2 * CH], m2b_s[:, 0:1])
        nc.vector.tensor_scalar(out=smf[:], in0=smf[:], scalar1=1.0 / L,
                                scalar2=None, op0=A.mult)
        mx = pP.tile([1, 1], FP32, tag="mx")
        nc.vector.reduce_max(mx[:], smf[:], axis=mybir.AxisListType.X)
        nc.vector.tensor_scalar(out=mx[:], in0=mx[:], scalar1=-1.0,
                                scalar2=None, op0=A.mult)
        nc.scalar.activation(smf[:], smf[:], F.Exp, bias=mx[0:1, 0:1])
        sm_s = pP.tile([1, 1], FP32, tag="sm_s")
        nc.vector.reduce_sum(sm_s[:], smf[:], axis=mybir.AxisListType.X)
        nc.vector.reciprocal(sm_s[:], sm_s[:])
        nc.vector.tensor_scalar(out=smf[:], in0=smf[:], scalar1=sm_s[0:1, 0:1],
                                scalar2=None, op0=A.mult)
        sm1 = pP.tile([CH, 1], FP32, tag="sm1")
        sm2 = pP.tile([CH, 1], FP32, tag="sm2")
        nc.sync.dma_start(sm1[:, 0:1], smf[0:1, 0:CH])
        nc.sync.dma_start(sm2[:, 0:1], smf[0:1, CH:2 * CH])
        o2f = rot.tile([CH, L], BF16, tag="pb")
        nc.sync.dma_start(o2f[0:72, :], Y2a[:])
        nc.sync.dma_start(o2f[72:96, :], lowc[:])
        o2t = pP.tile([CH, L], BF16, tag="o2t")
        yc = pP.tile([CH, L], BF16, tag="yc")
        outt = pP.tile([COUT, L], FP32, tag="outt")
        for c0, c1 in MM_CHUNKS:
            nc.vector.tensor_scalar(out=o2t[:, c0:c1], in0=o2f[:, c0:c1],
                                    scalar1=sm2[:, 0:1], scalar2=None, op0=A.mult)
            nc.vector.scalar_tensor_tensor(out=yc[:, c0:c1], in0=Y1[:, c0:c1],
                                           scalar=sm1[:, 0:1], in1=o2t[:, c0:c1],
                                           op0=A.mult, op1=A.add)
            ps = post_ps.tile([COUT, 512], FP32, tag="ps")
            nc.tensor.matmul(ps[:, :c1 - c0], cv('finT'), yc[:, c0:c1], start=True, stop=True)
            nc.scalar.activation(outt[:, c0:c1], ps[:, :c1 - c0], F.Identity, bias=cv('finb'))
            nc.sync.dma_start(out_d[:, c0:c1], outt[:, c0:c1])
        po.close()
        glob.close()
    split_multi_waits(nc, max_waits=1)
    return nc


# =============================== host side ==================================

def prep_core_inputs(inputs, b, half):
    import ml_dtypes
    f32 = np.float32
    bf16 = ml_dtypes.bfloat16
    d0 = half * HH

    def asf(a):
        return np.asarray(a, f32)

    bnscale = asf(inputs['bn_g']) / np.sqrt(np.float32(1.0 + 1e-5))
    w1 = asf(inputs['conv1_w'])[:, :, 0, 0] * bnscale[:, None]
    b1 = asf(inputs['conv1_b']) * bnscale + asf(inputs['bn_b'])

    def diag9(w, nch):
        out = np.zeros((nch, 9 * nch), f32)
        w = asf(w)
        for tap in range(9):
            dy, dx = tap // 3, tap % 3
            blk = out[:, tap * nch:(tap + 1) * nch]
            np.fill_diagonal(blk, w[:, 0, dy, dx])
        return out

    sscd = diag9(inputs['ss_conv_w'], DI)        # (192, 9*192)
    sc0 = np.zeros((128, 9 * 128), f32)
    sc1 = np.zeros((64, 9 * 64), f32)
    for tap in range(9):
        blk = sscd[:, tap * DI:(tap + 1) * DI]
        sc0[:, tap * 128:(tap + 1) * 128] = blk[0:128, 0:128]
        sc1[:, tap * 64:(tap + 1) * 64] = blk[128:192, 128:192]

    sel = np.zeros((DI, HH), f32)
    sel[np.arange(d0, d0 + HH), np.arange(HH)] = 1.0

    xp = asf(inputs['ss_xproj_w'])               # (4, 38, 192)
    xpTa = np.zeros((128, K4 * 38), f32)
    xpTb = np.zeros((64, K4 * 38), f32)
    for k in range(K4):
        xpT = xp[k].T                            # (192, 38)
        xpTa[:, k * 38:(k + 1) * 38] = xpT[0:128]
        xpTb[:, k * 38:(k + 1) * 38] = xpT[128:192]

    dtw = asf(inputs['ss_dt_w'])
    dtwT = np.zeros((R, K4 * HH), f32)
    for k in range(K4):
        dtwT[:, k * HH:(k + 1) * HH] = dtw[k][d0:d0 + HH, :].T

    dtb_full = asf(inputs['ss_dt_b'])
    Alog = asf(inputs['ss_Alog']).reshape(K4, DI, N)
    Dv = asf(inputs['ss_D']).reshape(K4, DI)
    dtb_p = np.zeros((128, NT), f32)
    Ap = np.zeros((128, NT * N), f32)
    for (j, o0, o1, k, dd0, dd1) in SECTIONS:
        dtb_p[o0:o1, j] = dtb_full[k, d0 + dd0:d0 + dd1]
        Ap[o0:o1, j * N:(j + 1) * N] = -np.exp(Alog[k, d0 + dd0:d0 + dd1])
    Dsum = Dv[:, d0:d0 + HH].sum(0)[:, None]

    gw = asf(inputs['gwc_w'])
    gT = np.zeros((24, 9 * CH), f32)
    for tap in range(9):
        dy, dx = tap // 3, tap % 3
        blk = np.zeros((24, CH), f32)
        blk[0:12, 0:48] = gw[0:48, :, dy, dx].T
        blk[12:24, 48:96] = gw[48:96, :, dy, dx].T
        gT[:, tap * CH:(tap + 1) * CH] = blk

    owT = asf(inputs['ss_out_w']).T              # (192, 96)
    outn_g = asf(inputs['ss_outn_g'])
    outn_b = asf(inputs['ss_outn_b'])

    vals32 = {
        'b1': b1[:, None],
        'linb': asf(inputs['lin_b'])[:, None],
        'dw1b': asf(inputs['dw1_b'])[:, None],
        'dw2b': asf(inputs['dw2_b'])[:, None],
        'scb0': asf(inputs['ss_conv_b'])[0:128, None],
        'scb1': asf(inputs['ss_conv_b'])[128:192, None],
        'dtb': dtb_p, 'Ap': Ap, 'Dsum': Dsum,
        'outngA': outn_g[0:96, None], 'outngB': outn_g[96:192, None],
        'outnbA': outn_b[0:96, None], 'outnbB': outn_b[96:192, None],
        'ag1b': asf(inputs['ag1_b'])[:, None],
        'ag2b': asf(inputs['ag2_b'])[:, None],
        'lng': asf(inputs['ln_g'])[:, None],
        'lnb': asf(inputs['ln_b'])[:, None],
        'gwcb': asf(inputs['gwc_b'])[:, None],
        'finb': asf(inputs['fin_b'])[:, None],
    }
    valsbf = {
        'w1T': w1.T,
        'linT': asf(inputs['lin_w']).T,
        'dw1dg': diag9(inputs['dw1_w'], CH),
        'dw2dg': diag9(inputs['dw2_w'], CH),
        'inwT': asf(inputs['ss_in_w']).T,        # (96, 384) full z
        'sc0dg': sc0, 'sc1dg': sc1,
        'sel0': sel[0:128], 'sel1': sel[128:192],
        'xpTa': xpTa, 'xpTb': xpTb,
        'dtwT': dtwT,
        'ident': np.eye(128, dtype=f32),
        'outwTa': owT[0:96], 'outwTb': owT[96:192],
        'ag1T': asf(inputs['ag1_w'])[:, :, 0, 0].T,
        'ag2T': asf(inputs['ag2_w'])[:, :, 0, 0].T,
        'sq1T': asf(inputs['sq1_w'])[:, :, 0, 0].T,
        'sq2T': asf(inputs['sq2_w'])[:, :, 0, 0].T,
        'gwcT': gT,
        'pw1T': asf(inputs['pwc1_w'])[:, :, 0, 0].T,
        'pw2T': asf(inputs['pwc2_w'])[:, :, 0, 0].T,
        'finT': asf(inputs['fin_w']).T,
        'bcv': np.concatenate([np.full((1, 96), 1.0 / DI, f32),
                               np.full((1, 96), 1.0 / CH, f32),
                               np.ones((1, 96), f32)], axis=1),
    }

    blob32 = np.zeros((128, W32), f32)
    for nm, p, c in CONSTS_F32:
        o = OFF32[nm][0]
        v = vals32[nm]
        assert v.shape == (p, c), (nm, v.shape, (p, c))
        blob32[0:p, o:o + c] = v
    blobbf = np.zeros((128, WBF), bf16)
    for nm, p, c in CONSTS_BF16:
        o = OFFBF[nm][0]
        v = valsbf[nm]
        assert v.shape == (p, c), (nm, v.shape, (p, c))
        blobbf[0:p, o:o + c] = v.astype(bf16)

    return {
        'x': np.ascontiguousarray(asf(inputs['x'])[b].reshape(CIN, L).astype(bf16)),
        'c32': blob32,
        'cbf': np.ascontiguousarray(blobbf),
    }


_NC_CACHE = {}


def get_nc():
    if 'nc' not in _NC_CACHE:
        _NC_CACHE['nc'] = build_nc()
    return _NC_CACHE['nc']


def kernel(**inputs):
    from concourse.bass_utils import run_bass_kernel_spmd
    nc = get_nc()
    in_maps = [prep_core_inputs(inputs, c // 2, c % 2) for c in range(8)]
    res = run_bass_kernel_spmd(nc, in_maps, core_ids=list(range(8)))
    out = np.zeros((B_, COUT, H, W), np.float32)
    for b in range(B_):
        out[b] = res.results[2 * b]['out'].reshape(COUT, H, W)
    return out


# revision 65
# speedup vs baseline: 1.0810x; 1.0069x over previous
"""Self-contained Trainium2 Bass kernel for the CR-VSS block (8 cores)."""

# ---- TileContext drain-wait patch (walrus 1-wait limit) ----
"""Patch TileContext._drain_and_barrier: the axon-client walrus rejects
instructions carrying >2 sem waits ("Too many sync wait commands" in
setupSyncWait for CTRL structs). Redistribute the exit-drain's waits across
preceding SP nop instructions, each carrying at most MAX_WAITS."""
from concourse.tile import TileContext, ScopedClock

MAX_WAITS = 1


def _patched_drain_and_barrier(self, tick_clock, wait_clock):
    nc = self.nc
    drain_inst = nc.sync.drain()
    wait_clock.add_sem_waits(
        drain_inst.ins, ScopedClock({None: tick_clock.global_clock})
    )

    waits = list(drain_inst.ins.sync_info.on_wait or [])
    if len(waits) > MAX_WAITS:
        bb = nc.cur_bb.bb
        assert bb.instructions[-1] is drain_inst.ins
        # strip waits from the drain, re-emit them on nop carriers
        drain_inst.ins.sync_info.on_wait = waits[:0]
        carriers = []
        import concourse.mybir as mybir
        for i in range(0, len(waits), MAX_WAITS):
            nop = nc.sync.nop(nofuse=True)
            nop.ins.sync_info = mybir.SyncInfo(
                on_wait=waits[i:i + MAX_WAITS], on_update=[]
            )
            carriers.append(nop.ins)
        # move carriers before the drain
        insts = list(bb.instructions)
        assert insts[-len(carriers) - 1] is drain_inst.ins
        reordered = insts[:-len(carriers) - 1] + insts[-len(carriers):] + [drain_inst.ins]
        while len(bb.instructions):
            bb.instructions.pop()
        for x in reordered:
            bb.instructions.append(x)

    nc.all_engine_barrier()
    assert self.sems is not None
    popped = nc._tile_sem_poison_stack.pop()
    assert popped is self._sem_poison
    nc.clear_and_free_semaphores(list(self.sems.allocated().values()))
    nc.all_engine_barrier()


def apply():
    TileContext._drain_and_barrier = _patched_drain_and_barrier


def split_multi_waits(nc, max_waits=1):
    """Post-pass: walrus CTRL codegen rejects instructions with more than
    one sem wait. Move extra waits onto same-engine NoOp carriers."""
    import concourse.mybir as mybir
    for f in nc.m.functions:
        for bb in f.blocks:
            insts = list(bb.instructions)
            out = []
            changed = False
            for ins in insts:
                si = ins.sync_info
                if si is not None and si.on_wait and len(si.on_wait) > max_waits:
                    waits = list(si.on_wait)
                    for i, w in enumerate(waits[max_waits:]):
                        nop = mybir.InstNoOp.__new__(
                            mybir.InstNoOp, name=f"{ins.name}-xw{i}", ins=[], outs=[])
                        nop.engine = ins.engine
                        nop.sync_info = mybir.SyncInfo(on_wait=[w], on_update=[])
                        out.append(nop)
                    ins.sync_info = mybir.SyncInfo(
                        on_wait=waits[:max_waits],
                        on_update=list(si.on_update or []))
                    changed = True
                out.append(ins)
            if changed:
                while len(bb.instructions):
                    bb.instructions.pop()
                for x in out:
                    bb.instructions.append(x)

apply()

# ---- kernel ----
"""Trainium2 Bass kernel for nn_CR_VSS (VSS block with SS2D selective scan).

Sharding: 8 cores = 4 samples x 2 d_inner-halves. Each core runs the full
pre-stage for its sample, scans its 96-channel d-half across all 4
cross-scan directions (packed into 3x128-partition tiles), then the pair
exchanges y-halves with ONE AllGather; LN + out-proj + post-stage run
locally (z is computed full-width in the in-proj so no second collective).

Scan: h_t = exp(A*delta_t)*h_{t-1} + delta_t*u_t*B_t per (k,d,n) via
tensor_tensor_scan; n in groups of 4 with batched B/C partition-broadcast
DMAs (double-buffered); y accumulated over n with identity-lhsT PSUM
matmuls, merged into pixel-order ysum straight from PSUM per t-chunk.
"""
import numpy as np
from contextlib import ExitStack

import concourse.bass as bass
import concourse.mybir as mybir

F = mybir.ActivationFunctionType
A = mybir.AluOpType
FP32 = mybir.dt.float32
BF16 = mybir.dt.bfloat16

B_, CIN, CH, COUT, H, W = 4, 96, 96, 96, 48, 48
DI, N, R, K4 = 192, 16, 6, 4
L = H * W               # 2304
HH = 96                 # d-half per core
NT = 3                  # packed (k,d) tiles: 4*96 = 384 = 3*128
HP = 50
LP = 2500
TC = 768                # scan t-chunk (16 rows of 48)
TCH = [(0, 768), (768, 1536), (1536, 2304)]
NG = 4                  # scan n-group (broadcast batch)

# packed (k,d) rows -> (tile j, offset): sections (j, o0, o1, k, d0, d1).
# Section offsets are all 0/32/64 so PE matmuls can write them directly.
SECTIONS = [
    (0, 0, 32, 1, 0, 32),
    (0, 32, 128, 0, 0, 96),
    (1, 0, 64, 1, 32, 96),
    (1, 64, 128, 2, 0, 64),
    (2, 0, 32, 2, 64, 96),
    (2, 32, 128, 3, 0, 96),
]

MM_CHUNKS = [(0, 512), (512, 1024), (1024, 1536), (1536, 2048), (2048, 2304)]
ROW_CHUNKS = [(0, 10), (10, 20), (20, 30), (30, 40), (40, 48)]
SUBS768 = [(0, 512), (512, 768)]
INW_BLOCKS = [(0, 128), (128, 256), (256, 384)]

REPLICA_GROUPS = [[0, 1], [2, 3], [4, 5], [6, 7]]

# ---- const blobs (shared layout between host packing and kernel views) ----
CONSTS_F32 = [
    ('b1', 96, 1), ('linb', 96, 1),
    ('dw1b', 96, 1), ('dw2b', 96, 1),
    ('scb0', 128, 1), ('scb1', 64, 1),
    ('dtb', 128, 3), ('Ap', 128, 48), ('Dsum', 96, 1),
    ('outngA', 96, 1), ('outngB', 96, 1), ('outnbA', 96, 1), ('outnbB', 96, 1),
    ('ag1b', 48, 1), ('ag2b', 96, 1), ('lng', 96, 1), ('lnb', 96, 1),
    ('gwcb', 96, 1), ('finb', 96, 1),
]
CONSTS_BF16 = [
    ('w1T', 96, 96), ('linT', 96, 96),
    ('dw1dg', 96, 864), ('dw2dg', 96, 864),
    ('inwT', 96, 384),
    ('sc0dg', 128, 1152), ('sc1dg', 64, 576),
    ('sel0', 128, 96), ('sel1', 64, 96),
    ('xpTa', 128, 152), ('xpTb', 64, 152),
    ('dtwT', 6, 384),
    ('ident', 128, 128),
    ('outwTa', 96, 96), ('outwTb', 96, 96),
    ('ag1T', 96, 48), ('ag2T', 48, 96),
    ('sq1T', 48, 24), ('sq2T', 48, 24),
    ('gwcT', 24, 864), ('pw1T', 24, 96), ('pw2T', 24, 72),
    ('finT', 96, 96), ('bcv', 1, 288),
]

OFF32 = {}
_o = 0
for _nm, _p, _c in CONSTS_F32:
    OFF32[_nm] = (_o, _p, _c)
    _o += _c
W32 = _o
OFFBF = {}
_o = 0
for _nm, _p, _c in CONSTS_BF16:
    OFFBF[_nm] = (_o, _p, _c)
    _o += _c
WBF = _o


def build_nc():
    nc = bass.Bass(trn_type="TRN2", num_devices=8)

    x_d = nc.dram_tensor("x", [CIN, L], BF16, kind="ExternalInput")
    c32_d = nc.dram_tensor("c32", [128, W32], FP32, kind="ExternalInput")
    cbf_d = nc.dram_tensor("cbf", [128, WBF], BF16, kind="ExternalInput")
    out_d = nc.dram_tensor("out", [COUT, L], FP32, kind="ExternalOutput")

    B_dram = nc.dram_tensor("B_dram", [K4 * N, L], BF16)
    C_dram = nc.dram_tensor("C_dram", [K4 * N, L], BF16)
    y_dram = nc.dram_tensor("y_dram", [HH, L], BF16)
    yg_dram = nc.dram_tensor("yg_dram", [DI, L], BF16)
    st_dram = nc.dram_tensor("st_dram", [2, L], BF16)

    def hw(ap):
        return ap.rearrange("p (h w) -> p h w", h=H)

    def hwp(ap):
        return ap.rearrange("p (h w) -> p h w", h=HP)

    def whv(ap):
        return ap.rearrange("p (h w) -> p w h", h=H)

    with TileContext(nc) as tc:
        glob = ExitStack()
        cst = glob.enter_context(tc.tile_pool(name="cst", bufs=1))
        lngA = glob.enter_context(tc.tile_pool(name="lngA", bufs=1))

        cst32 = cst.tile([128, W32], FP32, tag="cst32")
        cstbf = cst.tile([128, WBF], BF16, tag="cstbf")
        nc.sync.dma_start(cst32[:], c32_d[:])
        nc.sync.dma_start(cstbf[:], cbf_d[:])

        def cvc(nm, a0=0, a1=None, p0=0, p1=None):
            d, tile = (OFF32, cst32) if nm in OFF32 else (OFFBF, cstbf)
            o, p, c = d[nm]
            if a1 is None:
                a1 = c
            if p1 is None:
                p1 = p
            return tile[p0:p1, o + a0:o + a1]

        cv = cvc

        ones96 = cst.tile([HH, 1], BF16, tag="ones96")
        nc.vector.memset(ones96[:], 1.0)

        # long-lived across phases
        z0 = lngA.tile([128, L], BF16, tag="z0")     # z rows 0:128
        z1 = lngA.tile([64, L], BF16, tag="z1")      # z rows 128:192
        x2 = lngA.tile([CH, L], BF16, tag="x2")
        lngB = ExitStack()
        lngB_p = lngB.enter_context(tc.tile_pool(name="lngB_p", bufs=1))
        xch = lngB_p.tile([HH, L], BF16, tag="xch")
        dp = [lngB_p.tile([128, L], BF16, tag=f"dp{j}", name=f"dp{j}") for j in range(NT)]
        # xsp holds packed scan-order xs, overwritten in place with delta*u
        xsp = [lngB_p.tile([128, L], BF16, tag=f"xsp{j}", name=f"xsp{j}") for j in range(NT)]
        ysum = lngB_p.tile([HH, L], BF16, tag="ysum")

        # ================= pre-stage =================
        pre = ExitStack()
        pre_ps = pre.enter_context(tc.tile_pool(name="pre_ps", bufs=6, space="PSUM"))
        pA = pre.enter_context(tc.tile_pool(name="pA", bufs=1))
        pB = pre.enter_context(tc.tile_pool(name="pB", bufs=1))

        xt = pA.tile([CIN, L], BF16, tag="xt")
        nc.sync.dma_start(xt[:], x_d[:])

        # conv1x1 (+folded BN) + ReLU
        h1 = pA.tile([CH, L], BF16, tag="h1")
        for c0, c1 in MM_CHUNKS:
            ps = pre_ps.tile([CH, 512], FP32, tag="ps")
            nc.tensor.matmul(ps[:, :c1 - c0], cv('w1T'), xt[:, c0:c1], start=True, stop=True)
            nc.scalar.activation(h1[:, c0:c1], ps[:, :c1 - c0], F.Relu, bias=cv('b1'))
        # token linear
        h2 = pA.tile([CH, L], BF16, tag="h2")
        for c0, c1 in MM_CHUNKS:
            ps = pre_ps.tile([CH, 512], FP32, tag="ps")
            nc.tensor.matmul(ps[:, :c1 - c0], cv('linT'), h1[:, c0:c1], start=True, stop=True)
            nc.vector.tensor_scalar(out=h2[:, c0:c1], in0=ps[:, :c1 - c0],
                                    scalar1=cv('linb'), scalar2=None, op0=A.add)
        h2p = pA.tile([CH, LP], BF16, tag="h2p")
        nc.gpsimd.memset(h2p[:], 0.0)
        for (r0, r1) in ROW_CHUNKS:
            nc.vector.tensor_copy(hwp(h2p[:])[:, r0 + 1:r1 + 1, 1:49],
                                  hw(h2[:])[:, r0:r1, :])

        def dwconv(dst, src_p, dgname, biasname, nch):
            for (r0, r1) in ROW_CHUNKS:
                nr = r1 - r0
                ps = pre_ps.tile([128, 480], FP32, tag="ps")
                for tap in range(9):
                    dy, dx = tap // 3, tap % 3
                    rhs = hwp(src_p[:])[:, dy + r0:dy + r1, dx:dx + 48]
                    nc.tensor.matmul(ps[:nch, :nr * 48],
                                     cvc(dgname, tap * nch, (tap + 1) * nch),
                                     rhs, start=(tap == 0), stop=(tap == 8))
                nc.scalar.activation(dst[:, r0 * 48:r1 * 48], ps[:nch, :nr * 48],
                                     F.Silu, bias=cv(biasname))

        x1 = pB.tile([CH, L], BF16, tag="x1")
        dwconv(x1, h2p, 'dw1dg', 'dw1b', CH)

        # in-proj: xi (192) + FULL z (192)
        xi0 = pB.tile([128, L], BF16, tag="xi0")
        xi1 = pB.tile([64, L], BF16, tag="xi1")
        for mb, (m0, m1) in enumerate(INW_BLOCKS):
            for c0, c1 in MM_CHUNKS:
                ps = pre_ps.tile([128, 512], FP32, tag="ps")
                nc.tensor.matmul(ps[:m1 - m0, :c1 - c0], cvc('inwT', m0, m1),
                                 x1[:, c0:c1], start=True, stop=True)
                if mb == 0:
                    nc.vector.tensor_copy(xi0[:, c0:c1], ps[:128, :c1 - c0])
                elif mb == 1:
                    nc.scalar.copy(xi1[:, c0:c1], ps[0:64, :c1 - c0])
                    nc.scalar.copy(z0[0:64, c0:c1], ps[64:128, :c1 - c0])
                else:
                    nc.scalar.copy(z0[64:128, c0:c1], ps[0:64, :c1 - c0])
                    nc.scalar.copy(z1[0:64, c0:c1], ps[64:128, :c1 - c0])

        xi0p = pB.tile([128, LP], BF16, tag="xi0p")
        xi1p = pB.tile([64, LP], BF16, tag="xi1p")
        nc.gpsimd.memset(xi0p[:], 0.0)
        nc.gpsimd.memset(xi1p[:], 0.0)
        for (r0, r1) in ROW_CHUNKS:
            nc.vector.tensor_copy(hwp(xi0p[:])[:, r0 + 1:r1 + 1, 1:49],
                                  hw(xi0[:])[:, r0:r1, :])
            nc.vector.tensor_copy(hwp(xi1p[:])[:, r0 + 1:r1 + 1, 1:49],
                                  hw(xi1[:])[:, r0:r1, :])
        xc0 = pB.tile([128, L], BF16, tag="xc0")
        xc1 = pB.tile([64, L], BF16, tag="xc1")
        dwconv(xc0, xi0p, 'sc0dg', 'scb0', 128)
        dwconv(xc1, xi1p, 'sc1dg', 'scb1', 64)

        # d-half extraction + wh copy
        for c0, c1 in MM_CHUNKS:
            ps = pre_ps.tile([HH, 512], FP32, tag="ps")
            nc.tensor.matmul(ps[:, :c1 - c0], cv('sel0'), xc0[:, c0:c1], start=True, stop=False)
            nc.tensor.matmul(ps[:, :c1 - c0], cv('sel1'), xc1[:, c0:c1], start=False, stop=True)
            nc.vector.tensor_copy(xch[:, c0:c1], ps[:, :c1 - c0])
        xwhh = pB.tile([HH, L], BF16, tag="xwhh")
        for (t0, t1) in TCH:
            w0, w1 = t0 // 48, t1 // 48
            nc.vector.tensor_copy(hw(xwhh[:])[:, w0:w1, :],
                                  whv(xch[:])[:, w0:w1, :])

        # xproj (compact 38 rows: 0:6 dts, 6:22 B, 22:38 C) in scan order
        def xc_read(k, c0, c1):
            if k == 0:
                return (xc0[:, c0:c1], xc1[:, c0:c1])
            if k == 1:
                return (whv(xc0[:])[:, c0 // 48:c1 // 48, :],
                        whv(xc1[:])[:, c0 // 48:c1 // 48, :])
            if k == 2:
                return (xc0[:, L - c1:L - c0][:, ::-1],
                        xc1[:, L - c1:L - c0][:, ::-1])
            r0 = whv(xc0[:])[:, (L - c1) // 48:(L - c0) // 48, :][:, ::-1, ::-1]
            r1 = whv(xc1[:])[:, (L - c1) // 48:(L - c0) // 48, :][:, ::-1, ::-1]
            return (r0, r1)

        # row-chunk outer so all 4 directions' early columns finish first;
        # B/C are written to DRAM per scan chunk so ci=0 broadcasts can
        # start while xproj still works on later chunks.
        stage = [pB.tile([38, L], BF16, tag=f"stg{k}", name=f"stg{k}") for k in range(K4)]
        done_w = 0
        for ri, (rr0, rr1) in enumerate(ROW_CHUNKS):
            c0, c1 = rr0 * 48, rr1 * 48
            nf = c1 - c0
            for k in range(K4):
                ra, rb = xc_read(k, c0, c1)
                ps = pre_ps.tile([38, 480], FP32, tag="ps")
                nc.tensor.matmul(ps[:, :nf], cvc('xpTa', k * 38, (k + 1) * 38), ra,
                                 start=True, stop=False)
                nc.tensor.matmul(ps[:, :nf], cvc('xpTb', k * 38, (k + 1) * 38), rb,
                                 start=False, stop=True)
                nc.vector.tensor_copy(stage[k][:, c0:c1], ps[:, :nf])
            while done_w < len(TCH) and TCH[done_w][1] <= c1:
                t0, t1 = TCH[done_w]
                for k in range(K4):
                    nc.sync.dma_start(B_dram[k * N:(k + 1) * N, t0:t1],
                                      stage[k][6:22, t0:t1])
                    nc.sync.dma_start(C_dram[k * N:(k + 1) * N, t0:t1],
                                      stage[k][22:38, t0:t1])
                done_w += 1

        # delta: packed matmuls then softplus on full 128-partition tiles
        def mm_windows(a0, a1):
            if a0 == 0:
                return [(0, a1)]
            res = []
            x = a0
            while x < a1:
                if x % 64 == 32:
                    e = min(a1, x + 32)
                else:  # x == 64
                    e = min(a1, 128)
                res.append((x, e))
                x = e
            return res

        for (cc0, cc1) in MM_CHUNKS:
            cw = cc1 - cc0
            for j in range(NT):
                ex = pre_ps.tile([128, 512], FP32, tag="ps")
                for (jj, o0, o1, k, d0, d1) in SECTIONS:
                    if jj != j:
                        continue
                    for (w0, w1) in mm_windows(o0, o1):
                        dd0 = d0 + (w0 - o0)
                        dd1 = d0 + (w1 - o0)
                        nc.tensor.matmul(ex[w0:w1, :cw],
                                         cvc('dtwT', k * 96 + dd0, k * 96 + dd1),
                                         stage[k][0:6, cc0:cc1], start=True, stop=True)
                # softplus(x+b) = ln(1 + exp(x+b)) (no softplus act table on HW)
                ex2 = pre_ps.tile([128, 512], FP32, tag="ps")
                nc.scalar.activation(ex2[:, :cw], ex[:, :cw], F.Exp,
                                     bias=cvc('dtb', j, j + 1))
                nc.scalar.activation(dp[j][:, cc0:cc1], ex2[:, :cw], F.Ln, bias=1.0)

        # pack scan-order xs (Act copies handle partition shift + flips),
        # then overwrite in place with delta*u = dp*xs.
        # Act partition windows must not cross engine block boundaries on
        # EITHER side: allowed starts 0/32/64/96; a start-32 window may not
        # cross 64. split2 chops a shifted copy accordingly.
        def _legal_span(s):
            return 32 if s == 32 else 128 - s if s else 128

        def split2(o0, i0, ln):
            res = []
            x = 0
            while x < ln:
                step = min(ln - x, _legal_span(o0 + x), _legal_span(i0 + x))
                res.append((x, x + step))
                x += step
            return res

        for (t0, t1) in TCH:
            for (j, o0, o1, k, d0, d1) in SECTIONS:
                v = xwhh if k in (1, 3) else xch
                if k < 2:
                    # forward sections: contiguous rows, cheap DMA shift
                    nc.sync.dma_start(xsp[j][o0:o1, t0:t1], v[d0:d1, t0:t1])
                    continue
                for (w0, w1) in split2(o0, d0, o1 - o0):
                    nc.scalar.copy(xsp[j][o0 + w0:o0 + w1, t0:t1],
                                   v[d0 + w0:d0 + w1, ::-1][:, t0:t1])
            for j in range(NT):
                nc.vector.tensor_mul(xsp[j][:, t0:t1], dp[j][:, t0:t1],
                                     xsp[j][:, t0:t1])

        # branch-2 dwconv last: x2 is only needed in the post stage, so
        # this fills PE/Act slack once the scan inputs are queued
        dwconv(x2, h2p, 'dw2dg', 'dw2b', CH)

        pre.close()

        # ================= scan =================
        sc = ExitStack()
        bbp = sc.enter_context(tc.tile_pool(name="bbp", bufs=2))
        spool = sc.enter_context(tc.tile_pool(name="spool", bufs=2))
        scan_ps = sc.enter_context(tc.tile_pool(name="scan_ps", bufs=1, space="PSUM"))
        stp = sc.enter_context(tc.tile_pool(name="stp", bufs=1))
        state = [stp.tile([128, N], FP32, tag=f"st{j}", name=f"state{j}")
                 for j in range(NT)]
        yd = [stp.tile([HH, L], BF16, tag=f"yd{k}", name=f"yd{k}") for k in range(K4)]

        pending_drain = None
        for ci, (c0, c1) in enumerate(TCH):
            ypsum = [scan_ps.tile([128, TC], FP32, tag=f"yps{j}", name=f"yps{j}_{ci}")
                     for j in range(NT)]
            for g in range(K4):
                Bb = [bbp.tile([128, NG * TC], BF16, tag=f"Bb{j}", name=f"Bb{j}_{ci}_{g}")
                      for j in range(NT)]
                Cb = [bbp.tile([128, NG * TC], BF16, tag=f"Cb{j}", name=f"Cb{j}_{ci}_{g}")
                      for j in range(NT)]
                for (j, o0, o1, k, d0, d1) in SECTIONS:
                    nc.sync.dma_start(
                        Bb[j][o0:o1, :],
                        B_dram[k * N + NG * g:k * N + NG * g + NG, c0:c1]
                        .partition_broadcast(o1 - o0))
                    nc.scalar.dma_start(
                        Cb[j][o0:o1, :],
                        C_dram[k * N + NG * g:k * N + NG * g + NG, c0:c1]
                        .partition_broadcast(o1 - o0))
                if pending_drain is not None:
                    pending_drain()
                    pending_drain = None
                for n4 in range(NG):
                    n = NG * g + n4
                    for j in range(NT):
                        at = spool.tile([128, TC], BF16, tag=f"at{j}", name=f"at{j}")
                        nc.scalar.activation(at[:], dp[j][:, c0:c1], F.Exp,
                                             scale=cvc('Ap', j * N + n, j * N + n + 1))
                        bt = spool.tile([128, TC], BF16, tag=f"bt{j}", name=f"bt{j}", bufs=3)
                        nc.vector.tensor_mul(bt[:], xsp[j][:, c0:c1],
                                             Bb[j][:, n4 * TC:(n4 + 1) * TC])
                        ht = spool.tile([128, TC], BF16, tag=f"ht{j}", name=f"ht{j}", bufs=3)
                        if ci > 0:
                            # fold carry state into bt[0] so the scan can use
                            # the cheap zero-init form
                            nc.vector.scalar_tensor_tensor(
                                out=bt[:, 0:1], in0=at[:, 0:1],
                                scalar=state[j][:, n:n + 1], in1=bt[:, 0:1],
                                op0=A.mult, op1=A.add)
                        nc.vector.tensor_tensor_scan(ht[:], at[:], bt[:], 0.0,
                                                     A.mult, A.add)
                        if ci < 2:
                            nc.vector.tensor_copy(state[j][:, n:n + 1], ht[:, TC - 1:TC])
                        gt = spool.tile([128, TC], BF16, tag=f"gt{j}", name=f"gt{j}", bufs=3)
                        nc.vector.tensor_mul(gt[:], ht[:],
                                             Cb[j][:, n4 * TC:(n4 + 1) * TC])
                        for (s0, s1) in SUBS768:
                            nc.tensor.matmul(ypsum[j][:, s0:s1], cv('ident'),
                                             gt[:, s0:s1],
                                             start=(n == 0), stop=(n == N - 1))
            # drain this chunk's PSUM into per-direction scan-order tiles
            # (Act copies allow the partition shift). Deferred past the next
            # chunk's broadcast issue so the boundary doesn't stall Act.
            def _drain(yps=ypsum, cc0=c0, cc1=c1):
                for (j, o0, o1, k, d0, d1) in SECTIONS:
                    for (w0, w1) in split2(d0, o0, d1 - d0):
                        nc.scalar.copy(yd[k][d0 + w0:d0 + w1, cc0:cc1],
                                       yps[j][o0 + w0:o0 + w1, :])
            pending_drain = _drain
        if pending_drain is not None:
            pending_drain()
            pending_drain = None
        # merge directions into pixel order + D*u term
        tmp96 = stp.tile([HH, L], BF16, tag="tmp96")
        nc.vector.tensor_add(ysum[:], yd[0][:], yd[2][:, ::-1])
        nc.vector.tensor_add(tmp96[:], yd[1][:], yd[3][:, ::-1])
        nc.vector.tensor_add(ysum[:], ysum[:], whv(tmp96[:]))
        nc.vector.scalar_tensor_tensor(out=ysum[:], in0=xch[:], scalar=cv('Dsum'),
                                       in1=ysum[:], op0=A.mult, op1=A.add)
        nc.sync.dma_start(y_dram[:], ysum[:])
        nc.gpsimd.collective_compute(
            "AllGather", A.bypass, replica_groups=REPLICA_GROUPS,
            ins=[y_dram[:]], outs=[yg_dram[:]])
        sc.close()
        lngB.close()

        # ================= post =================
        po = ExitStack()
        post_ps = po.enter_context(tc.tile_pool(name="post_ps", bufs=6, space="PSUM"))
        pP = po.enter_context(tc.tile_pool(name="pP", bufs=1))
        rot = po.enter_context(tc.tile_pool(name="rot", bufs=5))

        # branch 2 + silu(z): no dep on the collective, runs under it
        g1 = rot.tile([48, L], BF16, tag="pb")
        for c0, c1 in MM_CHUNKS:
            ps = post_ps.tile([48, 512], FP32, tag="ps")
            nc.tensor.matmul(ps[:, :c1 - c0], cv('ag1T'), x2[:, c0:c1], start=True, stop=True)
            nc.scalar.activation(g1[:, c0:c1], ps[:, :c1 - c0], F.Relu, bias=cv('ag1b'))
        gat = rot.tile([CH, L], BF16, tag="pb")
        for c0, c1 in MM_CHUNKS:
            ps = post_ps.tile([CH, 512], FP32, tag="ps")
            nc.tensor.matmul(ps[:, :c1 - c0], cv('ag2T'), g1[:, c0:c1], start=True, stop=True)
            nc.scalar.activation(gat[:, c0:c1], ps[:, :c1 - c0], F.Sigmoid, bias=cv('ag2b'))
        x2g = pP.tile([CH, L], BF16, tag="x2g")
        nc.vector.tensor_mul(x2g[:], x2[:], gat[:])

        zsA = pP.tile([HH, L], BF16, tag="zsA")
        zsB = pP.tile([HH, L], BF16, tag="zsB")
        nc.scalar.activation(zsA[:], z0[0:96, :], F.Silu)
        nc.scalar.activation(zsB[0:32, :], z0[96:128, :], F.Silu)
        nc.scalar.activation(zsB[32:64, :], z1[0:32, :], F.Silu)
        nc.scalar.activation(zsB[64:96, :], z1[32:64, :], F.Silu)

        ygA = pP.tile([HH, L], BF16, tag="ygA")
        ygB = pP.tile([HH, L], BF16, tag="ygB")
        nc.sync.dma_start(ygA[:], yg_dram[0:HH, :])
        nc.sync.dma_start(ygB[:], yg_dram[HH:DI, :])

        sA = pP.tile([1, L], BF16, tag="sA")
        sB = pP.tile([1, L], BF16, tag="sB")
        sM = pP.tile([1, L], BF16, tag="sM")

        def ln_stats(cinv):
            # in: sA=raw sum, sB=raw sumsq; leaves rstd in sB (sA stays raw sum)
            with nc.allow_low_precision(reason="LN stats kept bf16; rel-err verified"):
                nc.scalar.activation(sM[:], sA[:], F.Square, scale=cinv)
                nc.vector.scalar_tensor_tensor(out=sB[:], in0=sB[:], scalar=cinv,
                                               in1=sM[:], op0=A.mult, op1=A.subtract)
                nc.vector.tensor_scalar(out=sB[:], in0=sB[:], scalar1=1e-5,
                                        scalar2=None, op0=A.add)
                nc.vector.reciprocal(sB[:], sB[:])
                nc.scalar.activation(sB[:], sB[:], F.Sqrt)

        # LayerNorm over full DI (local stats via ones-matmul), fully
        # chunk-pipelined behind the per-chunk yg reads
        ysqA = rot.tile([HH, L], BF16, tag="pb")
        ysqB = rot.tile([HH, L], BF16, tag="pb")
        for c, (c0, c1) in enumerate(MM_CHUNKS):
            nc.vector.tensor_mul(ysqA[:, c0:c1], ygA[:, c0:c1], ygA[:, c0:c1])
            nc.vector.tensor_mul(ysqB[:, c0:c1], ygB[:, c0:c1], ygB[:, c0:c1])
            ps = post_ps.tile([1, 512], FP32, tag="ps")
            nc.tensor.matmul(ps[:, :c1 - c0], ones96[:], ygA[:, c0:c1], start=True, stop=False)
            nc.tensor.matmul(ps[:, :c1 - c0], ones96[:], ygB[:, c0:c1], start=False, stop=True)
            nc.scalar.copy(sA[0:1, c0:c1], ps[:, :c1 - c0])
            ps2 = post_ps.tile([1, 512], FP32, tag="ps")
            nc.tensor.matmul(ps2[:, :c1 - c0], ones96[:], ysqA[:, c0:c1], start=True, stop=False)
            nc.tensor.matmul(ps2[:, :c1 - c0], ones96[:], ysqB[:, c0:c1], start=False, stop=True)
            nc.scalar.copy(sB[0:1, c0:c1], ps2[:, :c1 - c0])

        ln_stats(1.0 / DI)

        def apply_ln(pairs, bco):
            # pairs: list of (dst, src, gname, bname); bco: bcv column offset
            # holding 1/DI or 1/CH (folds the mean division into the
            # broadcast lhsT)
            for c0, c1 in MM_CHUNKS:
                cw = c1 - c0
                psm = post_ps.tile([HH, 512], FP32, tag="ps")
                nc.tensor.matmul(psm[:, :cw], cvc('bcv', bco, bco + HH),
                                 sA[:, c0:c1], start=True, stop=True)
                psr = post_ps.tile([HH, 512], FP32, tag="ps")
                nc.tensor.matmul(psr[:, :cw], cvc('bcv', 192, 192 + HH),
                                 sB[:, c0:c1], start=True, stop=True)
                for (dst, srct, gname, bname) in pairs:
                    nc.vector.tensor_sub(dst[:, c0:c1], srct[:, c0:c1],
                                         psm[:, :cw])
                    nc.vector.tensor_mul(dst[:, c0:c1], dst[:, c0:c1],
                                         psr[:, :cw])
                    nc.vector.tensor_scalar(out=dst[:, c0:c1], in0=dst[:, c0:c1],
                                            scalar1=cv(gname), scalar2=cv(bname),
                                            op0=A.mult, op1=A.add)

        ynA = rot.tile([HH, L], BF16, tag="pb")
        ynB = rot.tile([HH, L], BF16, tag="pb")
        apply_ln([(ynA, ygA, 'outngA', 'outnbA'),
                  (ynB, ygB, 'outngB', 'outnbB')], 0)

        # fused chunk pipeline: ygz -> out-proj -> yb -> ybsq -> LN2 stats
        gzA = rot.tile([HH, L], BF16, tag="pb")
        gzB = rot.tile([HH, L], BF16, tag="pb")
        x1o = pP.tile([CH, L], BF16, tag="x1o")
        yb = pP.tile([CH, L], BF16, tag="yb")
        ybsq = rot.tile([CH, L], BF16, tag="pb")
        for c0, c1 in MM_CHUNKS:
            cw = c1 - c0
            nc.vector.tensor_mul(gzA[:, c0:c1], ynA[:, c0:c1], zsA[:, c0:c1])
            nc.vector.tensor_mul(gzB[:, c0:c1], ynB[:, c0:c1], zsB[:, c0:c1])
            ps = post_ps.tile([CH, 512], FP32, tag="ps")
            nc.tensor.matmul(ps[:, :cw], cv('outwTa'), gzA[:, c0:c1],
                             start=True, stop=False)
            nc.tensor.matmul(ps[:, :cw], cv('outwTb'), gzB[:, c0:c1],
                             start=False, stop=True)
            nc.scalar.copy(x1o[:, c0:c1], ps[:, :cw])
            nc.vector.tensor_add(yb[:, c0:c1], x1o[:, c0:c1], x2g[:, c0:c1])
            nc.vector.tensor_mul(ybsq[:, c0:c1], yb[:, c0:c1], yb[:, c0:c1])
            ps1 = post_ps.tile([1, 512], FP32, tag="ps")
            nc.tensor.matmul(ps1[:, :cw], ones96[:], yb[:, c0:c1], start=True, stop=True)
            nc.scalar.copy(sA[0:1, c0:c1], ps1[:, :cw])
            ps2 = post_ps.tile([1, 512], FP32, tag="ps")
            nc.tensor.matmul(ps2[:, :cw], ones96[:], ybsq[:, c0:c1], start=True, stop=True)
            nc.scalar.copy(sB[0:1, c0:c1], ps2[:, :cw])
        ln_stats(1.0 / CH)
        ybn = pP.tile([CH, L], BF16, tag="ybn")
        apply_ln([(ybn, yb, 'lng', 'lnb')], 96)

        # CRM
        low_t = rot.tile([48, L], BF16, tag="pb")
        for c0, c1 in MM_CHUNKS:
            nc.sync.dma_start(low_t[:, c0:c1], ybn[48:96, c0:c1])
        upc = pP.tile([24, L], BF16, tag="upc")
        lowc = pP.tile([24, L], BF16, tag="lowc")
        m2cb = pP.tile([24, 5], FP32, tag="m2cb")
        _li = {c0: i for i, (c0, c1) in enumerate(MM_CHUNKS)}.get
        for c0, c1 in MM_CHUNKS:
            ps = post_ps.tile([24, 512], FP32, tag="ps")
            nc.tensor.matmul(ps[:, :c1 - c0], cv('sq1T'), ybn[0:48, c0:c1], start=True, stop=True)
            nc.scalar.copy(upc[:, c0:c1], ps[:, :c1 - c0])
            ps2 = post_ps.tile([24, 512], FP32, tag="ps")
            nc.tensor.matmul(ps2[:, :c1 - c0], cv('sq2T'), low_t[:, c0:c1], start=True, stop=True)
            nc.scalar.activation(lowc[:, c0:c1], ps2[:, :c1 - c0], F.Identity,
                                 accum_out=m2cb[:, _li(c0):_li(c0) + 1])
        upcp = pP.tile([24, LP], BF16, tag="upcp")
        nc.gpsimd.memset(upcp[:], 0.0)
        nc.vector.tensor_copy(hwp(upcp[:])[:, 1:49, 1:49], hw(upc[:]))
        Y1 = pP.tile([CH, L], BF16, tag="Y1")
        m1c = pP.tile([CH, 5], FP32, tag="m1c")
        for ri, (r0, r1) in enumerate(ROW_CHUNKS):
            nr = r1 - r0
            ps = post_ps.tile([CH, 480], FP32, tag="ps")
            for tap in range(9):
                dy, dx = tap // 3, tap % 3
                rhs = hwp(upcp[:])[:, dy + r0:dy + r1, dx:dx + 48]
                nc.tensor.matmul(ps[:, :nr * 48], cvc('gwcT', tap * CH, (tap + 1) * CH),
                                 rhs, start=(tap == 0), stop=False)
            nc.tensor.matmul(ps[:, :nr * 48], cv('pw1T'), upc[:, r0 * 48:r1 * 48],
                             start=False, stop=True)
            nc.scalar.activation(Y1[:, r0 * 48:r1 * 48], ps[:, :nr * 48],
                                 F.Identity, bias=cv('gwcb'),
                                 accum_out=m1c[:, ri:ri + 1])
        Y2a = pP.tile([72, L], BF16, tag="Y2a")
        m2ca = pP.tile([72, 5], FP32, tag="m2ca")
        for ri, (c0, c1) in enumerate(MM_CHUNKS):
            ps = post_ps.tile([72, 512], FP32, tag="ps")
            nc.tensor.matmul(ps[:, :c1 - c0], cv('pw2T'), lowc[:, c0:c1], start=True, stop=True)
            nc.scalar.activation(Y2a[:, c0:c1], ps[:, :c1 - c0], F.Identity,
                                 accum_out=m2ca[:, ri:ri + 1])
        m1 = pP.tile([CH, 1], FP32, tag="m1")
        m2a_s = pP.tile([72, 1], FP32, tag="m2a_s")
        m2b_s = pP.tile([24, 1], FP32, tag="m2b_s")
        nc.vector.reduce_sum(m1[:], m1c[:], axis=mybir.AxisListType.X)
        nc.vector.reduce_sum(m2a_s[:], m2ca[:], axis=mybir.AxisListType.X)
        nc.vector.reduce_sum(m2b_s[:], m2cb[:], axis=mybir.AxisListType.X)
        smf = pP.tile([1, 2 * CH], FP32, tag="smf")
        nc.sync.dma_start(smf[0:1, 0:CH], m1[:, 0:1])
        nc.sync.dma_start(smf[0:1, CH:CH + 72], m2a_s[:, 0:1])
        nc.sync.dma_start(smf[0:1, CH + 72:2 * CH], m2b_s[:, 0:1])
        nc.vector.tensor_scalar(out=smf[:], in0=smf[:], scalar1=1.0 / L,
                                scalar2=None, op0=A.mult)
        mx = pP.tile([1, 1], FP32, tag="mx")
        nc.vector.reduce_max(mx[:], smf[:], axis=mybir.AxisListType.X)
        nc.vector.tensor_scalar(out=mx[:], in0=mx[:], scalar1=-1.0,
                                scalar2=None, op0=A.mult)
        nc.scalar.activation(smf[:], smf[:], F.Exp, bias=mx[0:1, 0:1])
        sm_s = pP.tile([1, 1], FP32, tag="sm_s")
        nc.vector.reduce_sum(sm_s[:], smf[:], axis=mybir.AxisListType.X)
        nc.vector.reciprocal(sm_s[:], sm_s[:])
        nc.vector.tensor_scalar(out=smf[:], in0=smf[:], scalar1=sm_s[0:1, 0:1],
                                scalar2=None, op0=A.mult)
        sm1 = pP.tile([CH, 1], FP32, tag="sm1")
        sm2 = pP.tile([CH, 1], FP32, tag="sm2")
        nc.sync.dma_start(sm1[:, 0:1], smf[0:1, 0:CH])
        nc.sync.dma_start(sm2[:, 0:1], smf[0:1, CH:2 * CH])
        o2f = rot.tile([CH, L], BF16, tag="pb")
        nc.sync.dma_start(o2f[0:72, :], Y2a[:])
        nc.sync.dma_start(o2f[72:96, :], lowc[:])
        o2t = pP.tile([CH, L], BF16, tag="o2t")
        yc = pP.tile([CH, L], BF16, tag="yc")
        outt = pP.tile([COUT, L], FP32, tag="outt")
        for c0, c1 in MM_CHUNKS:
            nc.vector.tensor_scalar(out=o2t[:, c0:c1], in0=o2f[:, c0:c1],
                                    scalar1=sm2[:, 0:1], scalar2=None, op0=A.mult)
            nc.vector.scalar_tensor_tensor(out=yc[:, c0:c1], in0=Y1[:, c0:c1],
                                           scalar=sm1[:, 0:1], in1=o2t[:, c0:c1],
                                           op0=A.mult, op1=A.add)
            ps = post_ps.tile([COUT, 512], FP32, tag="ps")
            nc.tensor.matmul(ps[:, :c1 - c0], cv('finT'), yc[:, c0:c1], start=True, stop=True)
            nc.scalar.activation(outt[:, c0:c1], ps[:, :c1 - c0], F.Identity, bias=cv('finb'))
            nc.sync.dma_start(out_d[:, c0:c1], outt[:, c0:c1])
        po.close()
        glob.close()
    split_multi_waits(nc, max_waits=1)
    return nc


# =============================== host side ==================================

def prep_core_inputs(inputs, b, half):
    import ml_dtypes
    f32 = np.float32
    bf16 = ml_dtypes.bfloat16
    d0 = half * HH

    def asf(a):
        return np.asarray(a, f32)

    bnscale = asf(inputs['bn_g']) / np.sqrt(np.float32(1.0 + 1e-5))
    w1 = asf(inputs['conv1_w'])[:, :, 0, 0] * bnscale[:, None]
    b1 = asf(inputs['conv1_b']) * bnscale + asf(inputs['bn_b'])

    def diag9(w, nch):
        out = np.zeros((nch, 9 * nch), f32)
        w = asf(w)
        for tap in range(9):
            dy, dx = tap // 3, tap % 3
            blk = out[:, tap * nch:(tap + 1) * nch]
            np.fill_diagonal(blk, w[:, 0, dy, dx])
        return out

    sscd = diag9(inputs['ss_conv_w'], DI)        # (192, 9*192)
    sc0 = np.zeros((128, 9 * 128), f32)
    sc1 = np.zeros((64, 9 * 64), f32)
    for tap in range(9):
        blk = sscd[:, tap * DI:(tap + 1) * DI]
        sc0[:, tap * 128:(tap + 1) * 128] = blk[0:128, 0:128]
        sc1[:, tap * 64:(tap + 1) * 64] = blk[128:192, 128:192]

    sel = np.zeros((DI, HH), f32)
    sel[np.arange(d0, d0 + HH), np.arange(HH)] = 1.0

    xp = asf(inputs['ss_xproj_w'])               # (4, 38, 192)
    xpTa = np.zeros((128, K4 * 38), f32)
    xpTb = np.zeros((64, K4 * 38), f32)
    for k in range(K4):
        xpT = xp[k].T                            # (192, 38)
        xpTa[:, k * 38:(k + 1) * 38] = xpT[0:128]
        xpTb[:, k * 38:(k + 1) * 38] = xpT[128:192]

    dtw = asf(inputs['ss_dt_w'])
    dtwT = np.zeros((R, K4 * HH), f32)
    for k in range(K4):
        dtwT[:, k * HH:(k + 1) * HH] = dtw[k][d0:d0 + HH, :].T

    dtb_full = asf(inputs['ss_dt_b'])
    Alog = asf(inputs['ss_Alog']).reshape(K4, DI, N)
    Dv = asf(inputs['ss_D']).reshape(K4, DI)
    dtb_p = np.zeros((128, NT), f32)
    Ap = np.zeros((128, NT * N), f32)
    for (j, o0, o1, k, dd0, dd1) in SECTIONS:
        dtb_p[o0:o1, j] = dtb_full[k, d0 + dd0:d0 + dd1]
        Ap[o0:o1, j * N:(j + 1) * N] = -np.exp(Alog[k, d0 + dd0:d0 + dd1])
    Dsum = Dv[:, d0:d0 + HH].sum(0)[:, None]

    gw = asf(inputs['gwc_w'])
    gT = np.zeros((24, 9 * CH), f32)
    for tap in range(9):
        dy, dx = tap // 3, tap % 3
        blk = np.zeros((24, CH), f32)
        blk[0:12, 0:48] = gw[0:48, :, dy, dx].T
        blk[12:24, 48:96] = gw[48:96, :, dy, dx].T
        gT[:, tap * CH:(tap + 1) * CH] = blk

    owT = asf(inputs['ss_out_w']).T              # (192, 96)
    outn_g = asf(inputs['ss_outn_g'])
    outn_b = asf(inputs['ss_outn_b'])

    vals32 = {
        'b1': b1[:, None],
        'linb': asf(inputs['lin_b'])[:, None],
        'dw1b': asf(inputs['dw1_b'])[:, None],
        'dw2b': asf(inputs['dw2_b'])[:, None],
        'scb0': asf(inputs['ss_conv_b'])[0:128, None],
        'scb1': asf(inputs['ss_conv_b'])[128:192, None],
        'dtb': dtb_p, 'Ap': Ap, 'Dsum': Dsum,
        'outngA': outn_g[0:96, None], 'outngB': outn_g[96:192, None],
        'outnbA': outn_b[0:96, None], 'outnbB': outn_b[96:192, None],
        'ag1b': asf(inputs['ag1_b'])[:, None],
        'ag2b': asf(inputs['ag2_b'])[:, None],
        'lng': asf(inputs['ln_g'])[:, None],
        'lnb': asf(inputs['ln_b'])[:, None],
        'gwcb': asf(inputs['gwc_b'])[:, None],
        'finb': asf(inputs['fin_b'])[:, None],
    }
    valsbf = {
        'w1T': w1.T,
        'linT': asf(inputs['lin_w']).T,
        'dw1dg': diag9(inputs['dw1_w'], CH),
        'dw2dg': diag9(inputs['dw2_w'], CH),
        'inwT': asf(inputs['ss_in_w']).T,        # (96, 384) full z
        'sc0dg': sc0, 'sc1dg': sc1,
        'sel0': sel[0:128], 'sel1': sel[128:192],
        'xpTa': xpTa, 'xpTb': xpTb,
        'dtwT': dtwT,
        'ident': np.eye(128, dtype=f32),
        'outwTa': owT[0:96], 'outwTb': owT[96:192],
        'ag1T': asf(inputs['ag1_w'])[:, :, 0, 0].T,
        'ag2T': asf(inputs['ag2_w'])[:, :, 0, 0].T,
        'sq1T': asf(inputs['sq1_w'])[:, :, 0, 0].T,
        'sq2T': asf(inputs['sq2_w'])[:, :, 0, 0].T,
        'gwcT': gT,
        'pw1T': asf(inputs['pwc1_w'])[:, :, 0, 0].T,
        'pw2T': asf(inputs['pwc2_w'])[:, :, 0, 0].T,
        'finT': asf(inputs['fin_w']).T,
        'bcv': np.concatenate([np.full((1, 96), 1.0 / DI, f32),
                               np.full((1, 96), 1.0 / CH, f32),
                               np.ones((1, 96), f32)], axis=1),
    }

    blob32 = np.zeros((128, W32), f32)
    for nm, p, c in CONSTS_F32:
        o = OFF32[nm][0]
        v = vals32[nm]
        assert v.shape == (p, c), (nm, v.shape, (p, c))
        blob32[0:p, o:o + c] = v
    blobbf = np.zeros((128, WBF), bf16)
    for nm, p, c in CONSTS_BF16:
        o = OFFBF[nm][0]
        v = valsbf[nm]
        assert v.shape == (p, c), (nm, v.shape, (p, c))
        blobbf[0:p, o:o + c] = v.astype(bf16)

    return {
        'x': np.ascontiguousarray(asf(inputs['x'])[b].reshape(CIN, L).astype(bf16)),
        'c32': blob32,
        'cbf': np.ascontiguousarray(blobbf),
    }


_NC_CACHE = {}


def get_nc():
    if 'nc' not in _NC_CACHE:
        _NC_CACHE['nc'] = build_nc()
    return _NC_CACHE['nc']


def kernel(**inputs):
    from concourse.bass_utils import run_bass_kernel_spmd
    nc = get_nc()
    in_maps = [prep_core_inputs(inputs, c // 2, c % 2) for c in range(8)]
    res = run_bass_kernel_spmd(nc, in_maps, core_ids=list(range(8)))
    out = np.zeros((B_, COUT, H, W), np.float32)
    for b in range(B_):
        out[b] = res.results[2 * b]['out'].reshape(COUT, H, W)
    return out
